# revision 31
# baseline (speedup 1.0000x reference)
"""DGCNN-style GCN encoder on 8 TRN2 NeuronCores (bass/tile).

Data-parallel over batch: each core owns one sample (B=8). BatchNorm batch
statistics are all-reduced across the 8 cores so results match global-batch
BN exactly. kNN top-10, neighbor gathers, and all convs/BN/activations run
on-device; the host only reshapes/replicates weights.

Precision strategy: matmuls on the x -> x1 -> knn2 path use a compensated
fp32r (hi/lo split) scheme that recovers ~fp32 accuracy at fp32r speed
(conv1b uses plain fp32). This keeps the kNN neighbor selection stable
against the f32 reference. Everything downstream of x1 (conv2a/2b, block 3)
runs in bf16 (weights + activations), which the 2e-2 tolerance absorbs.

Performance notes (vs the first working version):
- edge loops process 1280-edge chunks (128 points x 10 nbrs) instead of 320
- neighbor gathers are indirect_copy, whose cost is ~28ns per index of Q7
  dispatch; conv2a gathers both x1 row-tiles in ONE pass by interleaving
  them as bf16 pairs (d=2 gather with doubled indices), halving index count
- gathers/loads are software-pipelined one-to-two chunks ahead, and the
  next phase's first loads + weight loads are issued before each BN
  AllReduce so DMA overlaps the collective
- y3a/y3b stay SBUF-resident in bf16 (no DRAM round-trip); y2a is spilled
  to DRAM in bf16 (half the traffic)
"""

import os
import sys
import json

B16 = "/nix/store/wxap7svlj45h0lfm31d1axjjnzyl6qsy-b16-bazel-unstable-cc-2026-05-04-9a3fa1f3-rt-2026-05-04-ade39e0a/lib/python3.13/site-packages"
if B16 not in sys.path:
    sys.path.insert(0, B16)
if "/opt/trn_rl_repo" not in sys.path:
    sys.path.insert(0, "/opt/trn_rl_repo")

import numpy as np
import concourse.bass as bass
import concourse.mybir as mybir
import concourse.tile as tile
from concourse.bass_utils import run_bass_kernel_spmd

dt = mybir.dt
AF = mybir.ActivationFunctionType
ALU = mybir.AluOpType
AX = mybir.AxisListType

N = 2048
KNN = 10
S = N * KNN            # 20480 edge positions
CB = 1280              # banded chunk: 128 points x 10 neighbors
NCHUNK = S // CB       # 16
CSL = [(0, 512), (512, 1024), (1024, 1280)]   # matmul col slices of a chunk
EPS = 1e-5
NCORES = 8
CNT = float(NCORES * S)      # BN count for edge tensors
CNT3 = float(NCORES * N)     # BN count for block-3 tensors


# ---------------------------------------------------------------- walrus fix
def _fix_bir_json(bir_json: bytes) -> bytes:
    """This container's walrus accepts only ONE sync-wait per instruction;
    hoist extra waits onto preceding EventSemaphore instructions."""
    m = json.loads(bir_json)
    for f in m["functions"]:
        for bb in f["blocks"]:
            out = []
            for i in bb["instructions"]:
                si = i.get("sync_info") or {}
                waits = si.get("on_wait") or []
                if len(waits) > 1:
                    for k, w in enumerate(waits[:-1]):
                        out.append({
                            "name": f"{i['name']}w{k}",
                            "engine": i["engine"],
                            "opcode": "EventSemaphore",
                            "ins": [], "outs": [],
                            "debug": i.get("debug"),
                            "sync_info": {"on_update": [], "on_wait": [w]},
                        })
                    i = dict(i)
                    si = dict(si)
                    si["on_wait"] = [waits[-1]]
                    i["sync_info"] = si
                out.append(i)
            bb["instructions"] = out
    return json.dumps(m).encode()


def _install_bassfix():
    import concourse.bass_utils as bu
    import concourse.bass2jax as b2j
    if getattr(bu, "_bassfix_installed", False):
        return
    real = bu.compile_bir_kernel

    def patched(bir_json, tmpdir, neff_name="file.neff"):
        return real(_fix_bir_json(bir_json), tmpdir, neff_name)

    bu.compile_bir_kernel = patched
    b2j.compile_bir_kernel = patched
    bu._bassfix_installed = True


# ------------------------------------------------------------- device helpers
def _round_split(nc, pool, src_ap, parts, width, name):
    """Return (hi, lo) float32r tiles for a f32 source AP [parts, width]."""
    hi = pool.tile([parts, width], dt.float32r, name=name + "_hi")
    lo = pool.tile([parts, width], dt.float32r, name=name + "_lo")
    nc.vector.tensor_copy(hi[:], src_ap)
    nc.vector.tensor_tensor(out=lo[:], in0=src_ap, in1=hi[:].bitcast(dt.float32),
                            op=ALU.subtract)
    return hi, lo


def _bn_coeffs(nc, pool, sums, sqs, g_ap, b_ap, rows, cnt, name):
    """sums/sqs: [rows,1] f32 (global). Returns (scale, bias) [rows,1] f32."""
    mean = pool.tile([rows, 1], dt.float32, name=name + "_mean")
    nc.vector.tensor_scalar_mul(mean[:], sums, 1.0 / cnt)
    e2 = pool.tile([rows, 1], dt.float32, name=name + "_e2")
    nc.vector.tensor_scalar_mul(e2[:], sqs, 1.0 / cnt)
    m2 = pool.tile([rows, 1], dt.float32, name=name + "_m2")
    nc.vector.tensor_tensor(out=m2[:], in0=mean[:], in1=mean[:], op=ALU.mult)
    var = pool.tile([rows, 1], dt.float32, name=name + "_var")
    nc.vector.tensor_tensor(out=var[:], in0=e2[:], in1=m2[:], op=ALU.subtract)
    epst = pool.tile([rows, 1], dt.float32, name=name + "_eps")
    nc.vector.memset(epst[:], float(EPS))
    std = pool.tile([rows, 1], dt.float32, name=name + "_std")
    nc.scalar.activation(std[:], var[:], AF.Sqrt, bias=epst[:])
    rstd = pool.tile([rows, 1], dt.float32, name=name + "_rstd")
    nc.vector.reciprocal(rstd[:], std[:])
    scale = pool.tile([rows, 1], dt.float32, name=name + "_scale")
    nc.vector.tensor_tensor(out=scale[:], in0=rstd[:], in1=g_ap, op=ALU.mult)
    ms = pool.tile([rows, 1], dt.float32, name=name + "_ms")
    nc.vector.tensor_tensor(out=ms[:], in0=mean[:], in1=scale[:], op=ALU.mult)
    bias = pool.tile([rows, 1], dt.float32, name=name + "_bias")
    nc.vector.tensor_tensor(out=bias[:], in0=b_ap, in1=ms[:], op=ALU.subtract)
    return scale, bias


def _allreduce(nc, dram, per, sums_list, sqs_list, rows_list, cpad, name):
    """Pack per-Mtile [rows,1] sum/sq into a DRAM [cpad,2] buffer, AllReduce
    across the 8 cores, unpack back into fresh [rows,1] tiles."""
    cin = dram.tile([cpad, 2], dt.float32, name=name + "_in")
    cout = dram.tile([cpad, 2], dt.float32, name=name + "_out")
    r0 = 0
    for s_, q_, rows in zip(sums_list, sqs_list, rows_list):
        nc.sync.dma_start(cin[r0:r0 + rows, 0:1], s_)
        nc.sync.dma_start(cin[r0:r0 + rows, 1:2], q_)
        r0 += rows
    if r0 < cpad:
        z = per.tile([cpad - r0, 2], dt.float32, name=name + "_z")
        nc.vector.memset(z[:], 0.0)
        nc.sync.dma_start(cin[r0:cpad, :], z[:])
    nc.gpsimd.collective_compute(
        "AllReduce", ALU.add, replica_groups=[list(range(NCORES))],
        ins=[cin[:].opt()], outs=[cout[:].opt()])
    outs = []
    r0 = 0
    for rows in rows_list:
        gs = per.tile([rows, 1], dt.float32, name=name + f"_gs{r0}")
        gq = per.tile([rows, 1], dt.float32, name=name + f"_gq{r0}")
        nc.sync.dma_start(gs[:], cout[r0:r0 + rows, 0:1])
        nc.sync.dma_start(gq[:], cout[r0:r0 + rows, 1:2])
        outs.append((gs, gq))
        r0 += rows
    return outs


def _acc_reduce(nc, per, acc, rows, name):
    """[rows, NCHUNK] accumulator -> [rows,1]."""
    out = per.tile([rows, 1], dt.float32, name=name)
    nc.vector.tensor_reduce(out=out[:], in_=acc, axis=AX.X, op=ALU.add)
    return out


def build_kernel(nc, tc):
    P = 128
    per = tc.alloc_tile_pool(name="persist", bufs=1)
    dram = tc.alloc_tile_pool(name="dram", bufs=1, space="DRAM")
    b1t = tc.alloc_tile_pool(name="b1t", bufs=2)
    oseg = tc.alloc_tile_pool(name="oseg", bufs=1)
    wld = tc.alloc_tile_pool(name="wld", bufs=2)
    ph1 = tc.alloc_tile_pool(name="ph1", bufs=1)

    ins = nc._ext_inputs
    x_d = ins["x"]

    # ---------------- load & prep ----------------
    tmp0 = tc.alloc_tile_pool(name="tmp0", bufs=1)      # until U1t/rhs built
    x = tmp0.tile([64, N], dt.float32, name="x")
    nc.sync.dma_start(x[:], x_d)
    xhi, xlo = _round_split(nc, tmp0, x[:], 64, N, "x")
    # stacks for compensated matmuls
    xstack = ph1.tile([P, N], dt.float32r, name="xstack")   # [xhi; xlo]
    nc.vector.tensor_copy(xstack[:64, :], xhi[:])
    nc.vector.tensor_copy(xstack[64:, :], xlo[:])
    lhsx = xstack                                             # [xhi; xlo]
    rhsx = ph1.tile([P, N], dt.float32r, name="rhsx")        # 2*[xlo; xhi]
    nc.vector.tensor_scalar_mul(rhsx[:64, :], xlo[:], 2.0)
    nc.vector.tensor_scalar_mul(rhsx[64:, :], xhi[:], 2.0)

    # sq1 = sum_c x^2 (compensated)
    xsq = tmp0.tile([64, N], dt.float32, name="xsq")
    nc.vector.tensor_tensor(out=xsq[:], in0=x[:], in1=x[:], op=ALU.mult)
    xsqhi, xsqlo = _round_split(nc, tmp0, xsq[:], 64, N, "xsq")
    ones_f = per.tile([128, 1], dt.float32, name="ones_f")
    nc.vector.memset(ones_f[:], 1.0)
    ones64 = per.tile([64, 1], dt.float32r, name="ones64")
    nc.vector.tensor_copy(ones64[:], ones_f[:64, :])
    # knn1 matmul operands (sq1 folded in chunk-wise)
    lhs_hi1 = ph1.tile([66, N], dt.float32r, name="lhs_hi1")
    nc.vector.tensor_copy(lhs_hi1[:64, :], xhi[:])
    ones2f = tmp0.tile([2, N], dt.float32, name="ones2f")
    nc.vector.memset(ones2f[:], 1.0)
    nc.vector.tensor_copy(lhs_hi1[64:66, :], ones2f[:])
    rhs_hi1 = ph1.tile([66, N], dt.float32r, name="rhs_hi1")
    nc.vector.tensor_scalar_mul(rhs_hi1[:64, :], xhi[:], 2.0)
    psq = tc.alloc_tile_pool(name="psq", bufs=1, space="PSUM")
    sq1p = psq.tile([1, N], dt.float32, name="sq1p", space="PSUM")
    for c in range(4):
        sl = slice(512 * c, 512 * (c + 1))
        nc.tensor.matmul(sq1p[:, sl], ones64[:], xsqhi[:, sl], start=True, stop=False)
        nc.tensor.matmul(sq1p[:, sl], ones64[:], xsqlo[:, sl], start=False, stop=True)
        sc_ = tmp0.tile([1, 512], dt.float32, name="sq1c")
        nc.scalar.copy(sc_[:], sq1p[:, sl])
        sh_ = tmp0.tile([1, 512], dt.float32r, name="sq1ch")
        nc.vector.tensor_copy(sh_[:], sc_[:])
        sl_ = tmp0.tile([1, 512], dt.float32r, name="sq1cl")
        nc.vector.tensor_tensor(out=sl_[:], in0=sc_[:], in1=sh_[:].bitcast(dt.float32),
                                op=ALU.subtract)
        nc.vector.tensor_scalar_mul(rhs_hi1[64:65, sl], sh_[:], -1.0)
        nl_ = tmp0.tile([1, 512], dt.float32r, name="sq1nl")
        nc.vector.tensor_scalar_mul(nl_[:], sl_[:], -1.0)
        nc.sync.dma_start(rhs_hi1[65:66, sl], nl_[:])
    psq.release()

    # conv1 weights
    w1aT = wld.tile([128, 595], dt.float32, name="wf")
    nc.sync.dma_start(w1aT[:64, :152], ins["w1aT"])
    w1aT_hi, w1aT_lo = _round_split(nc, per, w1aT[:64, :152], 64, 152, "w1aT")
    du1T = wld.tile([128, 595], dt.float32, name="wf")
    nc.sync.dma_start(du1T[:64, :152], ins["du1T"])
    du1T_hi, du1T_lo = _round_split(nc, per, du1T[:64, :152], 64, 152, "du1T")
    w1aX = per.tile([P, 152], dt.float32r, name="w1aX")      # [Wlo; Whi]
    nc.vector.tensor_copy(w1aX[:64, :], w1aT_lo[:])
    nc.vector.tensor_copy(w1aX[64:, :], w1aT_hi[:])
    du1X = per.tile([P, 152], dt.float32r, name="du1X")      # [dlo; dhi] pairs lhsx
    nc.vector.tensor_copy(du1X[:64, :], du1T_lo[:])
    nc.vector.tensor_copy(du1X[64:, :], du1T_hi[:])

    wkt1b = []  # conv1b fp32 weights, K-tiles
    for i, rows in enumerate((128, 24)):
        t = per.tile([rows, 181], dt.float32, name=f"w1bT_k{i}")
        nc.sync.dma_start(t[:], ins[f"w1bT_k{i}"])
        wkt1b.append(t)


    # U1[c, n] = sum_ci du1[ci, c] x[ci, n]   (compensated, natural layout)
    rows1a_ = (128, 24)
    u1 = [ph1.tile([r, N], dt.float32, name=f"u1_{i}") for i, r in enumerate(rows1a_)]
    pu = tc.alloc_tile_pool(name="pu", bufs=4, space="PSUM")
    for m, rows in enumerate(rows1a_):
        msl = slice(128 * m, 128 * m + rows)
        for cc in range(4):
            sl = slice(512 * cc, 512 * (cc + 1))
            up = pu.tile([rows, 512], dt.float32, name="up", space="PSUM")
            nc.tensor.matmul(up[:], du1T_hi[:, msl], xhi[:, sl], start=True, stop=False)
            nc.tensor.matmul(up[:], du1X[:, msl], xstack[:, sl], start=False, stop=True)
            nc.scalar.copy(u1[m][:, sl], up[:])
    pu.release()
    tmp0.release()

    # BN params
    def load_vec(nm, rows_list):
        tiles = []
        r0 = 0
        for i, rows in enumerate(rows_list):
            t = per.tile([rows, 1], dt.float32, name=f"{nm}_{i}")
            nc.sync.dma_start(t[:], ins[nm][r0:r0 + rows, :])
            tiles.append(t)
            r0 += rows
        return tiles

    g1a = load_vec("g1a", (128, 24)); b1a = load_vec("be1a", (128, 24))
    g1b = load_vec("g1b", (128, 53)); b1b = load_vec("be1b", (128, 53))
    g2a = load_vec("g2a", (128, 128, 128, 46)); b2a = load_vec("be2a", (128, 128, 128, 46))
    g2b = load_vec("g2b", (128,) * 4); b2b = load_vec("be2b", (128,) * 4)
    g3a = load_vec("g3a", (128,) * 4 + (83,)); b3a = load_vec("be3a", (128,) * 4 + (83,))
    g3b = load_vec("g3b", (128,) * 4); b3b = load_vec("be3b", (128,) * 4)

    # ---------------- kNN (shared machinery) ----------------
    idx1_d = dram.tile([16, P, KNN], dt.uint16, name="idx1_d")
    idx2_d = dram.tile([16, P, KNN], dt.uint16, name="idx2_d")

    def knn_pass(mm_emit, idx_dram, tag, after_tile=None, kp_bufs=2):
        """mm_emit(psum_slice, c) emits matmuls for columns 512c..512c+512."""
        kp = tc.alloc_tile_pool(name=f"knnp_{tag}", bufs=kp_bufs, space="PSUM")
        ks = tc.alloc_tile_pool(name=f"knns_{tag}", bufs=2)
        for t in range(16):
            pe = kp.tile([P, N], dt.float32, name="pe", space="PSUM")
            for c in range(4):
                mm_emit(pe, t, c)
            q = ks.tile([P, N], dt.float32, name="q")
            nc.scalar.copy(q[:], pe[:])
            v8 = ks.tile([P, 8], dt.float32, name="v8")
            i8 = ks.tile([P, 8], dt.uint16, name="i8")
            nc.vector.max(out=v8[:], in_=q[:])
            nc.vector.max_index(out=i8[:], in_max=v8[:], in_values=q[:])
            nc.vector.match_replace(out=q[:], in_to_replace=v8[:], in_values=q[:],
                                    imm_value=-1e30)
            v8b = ks.tile([P, 8], dt.float32, name="v8b")
            i8b = ks.tile([P, 8], dt.uint16, name="i8b")
            nc.vector.max(out=v8b[:], in_=q[:])
            nc.vector.max_index(out=i8b[:], in_max=v8b[:], in_values=q[:])
            idx = ks.tile([P, KNN], dt.uint16, name="idx")
            nc.vector.tensor_copy(idx[:, 0:7], i8[:, 1:8])
            nc.vector.tensor_copy(idx[:, 7:10], i8b[:, 0:3])
            nc.sync.dma_start(idx_dram[t], idx[:])
            if after_tile is not None:
                after_tile(t)
        kp.release()
        ks.release()

    def build_wrapped(idx_dram, tag):
        """DRAM flat idx (point-major) -> wrapped [128, S/16] uint16."""
        iw = per.tile([P, S // 16], dt.uint16, name=f"iw_{tag}")
        flat = idx_dram[:].rearrange("a p k -> (a p k)")
        nc.sync.dma_start(iw[0:16, :], flat.rearrange("(f p) -> p f", p=16))
        for g in range(1, 8):
            nc.sync.dma_start(iw[16 * g:16 * (g + 1), :], iw[0:16, :])
        return iw

    # knn1 matmuls
    def mm1(pe, t, c):
        msl = slice(128 * t, 128 * (t + 1))
        csl = slice(512 * c, 512 * (c + 1))
        nc.tensor.matmul(pe[:, csl], lhs_hi1[:, msl], rhs_hi1[:, csl],
                         start=True, stop=False)
        nc.tensor.matmul(pe[:, csl], lhsx[:, msl], rhsx[:, csl],
                         start=False, stop=True)

    # ---------------- block 1, interleaved with knn1 ----------------
    # y1a[(n,j)] = W1a @ (nb - cen) + W1b @ cen ; stats; keep for BN.
    # Each knn1 tile's top-k feeds its 4 stats chunks immediately, so the
    # gather-paced loop overlaps the DVE top-k of later tiles.
    y1a_d = [dram.tile([P, S], dt.float32, name="y1a_d0"),
             dram.tile([24, S], dt.float32, name="y1a_d1")]
    rows1a = (128, 24)
    sum1a = [ph1.tile([r, NCHUNK], dt.float32, name=f"s1a_{i}") for i, r in enumerate(rows1a)]
    sq1a = [ph1.tile([r, NCHUNK], dt.float32, name=f"q1a_{i}") for i, r in enumerate(rows1a)]
    knn_pass(mm1, idx1_d, "k1", kp_bufs=2)
    iw1 = build_wrapped(idx1_d, "k1")

    b1p = tc.alloc_tile_pool(name="b1p", bufs=2, space="PSUM")
    b1s = tc.alloc_tile_pool(name="b1s", bufs=3)

    b1q = {}
    def b1_gather(t):
        nbr = b1s.tile([P, CB], dt.float32, name="nbr")
        for g in range(2):
            nc.gpsimd.indirect_copy(out=nbr[:, 640 * g:640 * (g + 1)],
                                    data=xstack[:].bitcast(dt.float32),
                                    idxs=iw1[:, 80 * t + 40 * g:80 * t + 40 * (g + 1)],
                                    i_know_ap_gather_is_preferred=True)
        nbxt = b1s.tile([P, CB], dt.float32r, name="nbxt")
        nc.scalar.copy(nbxt[:], nbr[:])
        b1q[t] = nbxt

    b1_gather(0)
    for t in range(NCHUNK):
        if t + 1 < NCHUNK:
            b1_gather(t + 1)
        nbx = b1q.pop(t)[:]
        csl = slice(CB * t, CB * (t + 1))
        for m, rows in enumerate(rows1a):
            msl = slice(128 * m, 128 * m + rows)
            yp = b1p.tile([P, CB], dt.float32, name="yp", space="PSUM")
            for c0, c1 in CSL:
                nc.tensor.matmul(yp[:rows, c0:c1], w1aT_hi[:, msl],
                                 nbx[0:64, c0:c1], start=True, stop=False)
                nc.tensor.matmul(yp[:rows, c0:c1], w1aX[:, msl],
                                 nbx[:, c0:c1], start=False, stop=True)
            uview = u1[m][:, 128 * t:128 * (t + 1), None].to_broadcast(
                [rows, 128, KNN])
            ob = b1s.tile([rows, CB], dt.float32, name="ob1")
            nc.vector.scalar_tensor_tensor(
                out=ob[:].rearrange("p (n k) -> p n k", k=KNN),
                in0=yp[:rows, :].rearrange("p (n k) -> p n k", k=KNN), scalar=1.0,
                in1=uview, op0=ALU.mult, op1=ALU.add,
                accum_out=sum1a[m][:, t:t + 1])
            nc.sync.dma_start(y1a_d[m][:, csl], ob[:])
            tr = b1t.tile([rows, CB], dt.bfloat16, name="tr")
            nc.scalar.activation(tr[:], ob[:], AF.Square,
                                 accum_out=sq1a[m][:, t:t + 1])
    b1s.release()
    b1p.release()
    k2pool = tc.alloc_tile_pool(name="k2pool", bufs=1)

    s1 = [_acc_reduce(nc, per, sum1a[m][:], rows1a[m], f"s1r{m}") for m in range(2)]
    q1 = [_acc_reduce(nc, per, sq1a[m][:], rows1a[m], f"q1r{m}") for m in range(2)]

    # h1 load prefetch issued before the AllReduce so DMA overlaps it
    sumh1 = [ph1.tile([r, NCHUNK], dt.float32, name=f"sh1_{i}") for i, r in enumerate(rows1a)]
    rows1b = (128, 53)
    m1 = [k2pool.tile([r, N], dt.float32, name=f"m1_{i}") for i, r in enumerate(rows1b)]
    sq1b = [ph1.tile([r, NCHUNK], dt.float32, name=f"q1b_{i}") for i, r in enumerate(rows1b)]
    b2pp = tc.alloc_tile_pool(name="b2pp", bufs=2, space="PSUM")
    h1l = tc.alloc_tile_pool(name="h1l", bufs=2)

    h1q = {}
    def h1_load(c):
        csl = slice(CB * c, CB * (c + 1))
        pair = []
        for m, rows in enumerate(rows1a):
            ld = h1l.tile([rows, CB], dt.float32, name=f"h1ld{m}")
            nc.sync.dma_start(ld[:], y1a_d[m][:, csl])
            pair.append(ld)
        h1q[c] = pair

    h1_load(0)
    ar1 = _allreduce(nc, dram, per, [s1[0][:], s1[1][:]], [q1[0][:], q1[1][:]],
                     rows1a, 152, "ar1")
    sb1a = [_bn_coeffs(nc, per, ar1[m][0][:], ar1[m][1][:], g1a[m][:], b1a[m][:],
                       rows1a[m], CNT, f"bn1a{m}") for m in range(2)]
    for c in range(NCHUNK):
        if c + 1 < NCHUNK:
            h1_load(c + 1)
        csl = slice(CB * c, CB * (c + 1))
        h1c = h1q.pop(c)
        for m, rows in enumerate(rows1a):
            ld = h1c[m]
            nc.scalar.activation(ld[:], ld[:], AF.Prelu,
                                 bias=sb1a[m][1][:], scale=sb1a[m][0][:], alpha=0.2,
                                 accum_out=sumh1[m][:, c:c + 1])
        for m, rows in enumerate(rows1b):
            msl = slice(128 * m, 128 * m + rows)
            yp = b2pp.tile([P, CB], dt.float32, name="y1bp", space="PSUM")
            for c0, c1 in CSL:
                nc.tensor.matmul(yp[:rows, c0:c1], wkt1b[0][:, msl],
                                 h1c[0][:, c0:c1], start=True, stop=False)
                nc.tensor.matmul(yp[:rows, c0:c1], wkt1b[1][:, msl],
                                 h1c[1][:, c0:c1], start=False, stop=True)
            tr = b1t.tile([rows, CB], dt.bfloat16, name="tr")
            nc.scalar.activation(tr[:], yp[:rows, :], AF.Square,
                                 accum_out=sq1b[m][:, c:c + 1])
            nc.vector.tensor_reduce(out=m1[m][:, 128 * c:128 * (c + 1)],
                                    in_=yp[:rows, :].rearrange("p (n k) -> p n k", k=KNN),
                                    axis=AX.X, op=ALU.max)
    b2pp.release()
    h1l.release()

    # sum(y1b) = W1b' @ sum(h1)
    sh1 = [_acc_reduce(nc, per, sumh1[m][:], rows1a[m], f"sh1r{m}") for m in range(2)]
    ps1b = tc.alloc_tile_pool(name="ps1b", bufs=1, space="PSUM")
    s1b = []
    for m, rows in enumerate(rows1b):
        msl = slice(128 * m, 128 * m + rows)
        sp = ps1b.tile([rows, 1], dt.float32, name=f"s1bp{m}", space="PSUM")
        nc.tensor.matmul(sp[:], wkt1b[0][:, msl], sh1[0][:], start=True, stop=False)
        nc.tensor.matmul(sp[:], wkt1b[1][:, msl], sh1[1][:], start=False, stop=True)
        st = per.tile([rows, 1], dt.float32, name=f"s1b_{m}")
        nc.scalar.copy(st[:], sp[:])
        s1b.append(st)
    q1b = [_acc_reduce(nc, per, sq1b[m][:], rows1b[m], f"q1br{m}") for m in range(2)]
    ps1b.release()
    ar2 = _allreduce(nc, dram, per, [s1b[0][:], s1b[1][:]], [q1b[0][:], q1b[1][:]],
                     rows1b, 181, "ar2")
    sb1b = [_bn_coeffs(nc, per, ar2[m][0][:], ar2[m][1][:], g1b[m][:], b1b[m][:],
                       rows1b[m], CNT, f"bn1b{m}") for m in range(2)]

    # x1 = prelu(bn(m1)) -> hi/lo splits + sq2, all chunk-wise
    pk2op = tc.alloc_tile_pool(name="pk2op", bufs=1)   # knn2 operands (freed w/ k2pool)
    ones53 = per.tile([53, 1], dt.float32r, name="ones53")
    nc.vector.tensor_copy(ones53[:], ones_f[:53, :])
    ones128 = per.tile([128, 1], dt.float32r, name="ones128")
    nc.vector.tensor_copy(ones128[:], ones_f[:])
    x1hi = [per.tile([r, N], dt.float32r, name=f"x1s{m}_hi") for m, r in enumerate(rows1b)]
    x1lo = [pk2op.tile([r, N], dt.float32r, name=f"x1s{m}_lo") for m, r in enumerate(rows1b)]
    r2h_k1 = pk2op.tile([55, N], dt.float32r, name="r2h_k1")
    psq2 = tc.alloc_tile_pool(name="psq2", bufs=1, space="PSUM")
    sqt = tc.alloc_tile_pool(name="sqt", bufs=1)
    sq2p = psq2.tile([1, N], dt.float32, name="sq2p", space="PSUM")
    for c in range(4):
        sl = slice(512 * c, 512 * (c + 1))
        first = True
        for m, rows in enumerate(rows1b):
            x1c = sqt.tile([rows, 512], dt.float32, name="x1c")
            nc.scalar.activation(x1c[:], m1[m][:, sl], AF.Prelu, bias=sb1b[m][1][:],
                                 scale=sb1b[m][0][:], alpha=0.2)
            nc.vector.tensor_copy(x1hi[m][:, sl], x1c[:])
            nc.vector.tensor_tensor(out=x1lo[m][:, sl], in0=x1c[:],
                                    in1=x1hi[m][:, sl].bitcast(dt.float32),
                                    op=ALU.subtract)
            t = sqt.tile([rows, 512], dt.float32, name="x1sqc")
            nc.vector.tensor_tensor(out=t[:], in0=x1c[:], in1=x1c[:], op=ALU.mult)
            h = sqt.tile([rows, 512], dt.float32r, name="x1sqh")
            nc.vector.tensor_copy(h[:], t[:])
            l = sqt.tile([rows, 512], dt.float32r, name="x1sql")
            nc.vector.tensor_tensor(out=l[:], in0=t[:], in1=h[:].bitcast(dt.float32),
                                    op=ALU.subtract)
            on = ones128 if rows == 128 else ones53
            nc.tensor.matmul(sq2p[:, sl], on[:], h[:], start=first, stop=False)
            first = False
            nc.tensor.matmul(sq2p[:, sl], on[:], l[:],
                             start=False, stop=(m == 1))
        sc_ = sqt.tile([1, 512], dt.float32, name="sq2c")
        nc.scalar.copy(sc_[:], sq2p[:, sl])
        sh_ = sqt.tile([1, 512], dt.float32r, name="sq2ch")
        nc.vector.tensor_copy(sh_[:], sc_[:])
        sl_ = sqt.tile([1, 512], dt.float32r, name="sq2cl")
        nc.vector.tensor_tensor(out=sl_[:], in0=sc_[:], in1=sh_[:].bitcast(dt.float32),
                                op=ALU.subtract)
        nh_ = sqt.tile([1, 512], dt.float32r, name="sq2nh")
        nc.vector.tensor_scalar_mul(nh_[:], sh_[:], -1.0)
        nc.sync.dma_start(r2h_k1[53:54, sl], nh_[:])
        nl_ = sqt.tile([1, 512], dt.float32r, name="sq2nl")
        nc.vector.tensor_scalar_mul(nl_[:], sl_[:], -1.0)
        nc.sync.dma_start(r2h_k1[54:55, sl], nl_[:])
    psq2.release()
    sqt.release()

    # knn2 operands: lhsT reuses x1hi/x1lo; only scaled rhs tiles are built
    l2h_k1 = pk2op.tile([55, N], dt.float32r, name="l2h_k1")    # x1hi 128:181 + 2 ones
    nc.vector.tensor_copy(l2h_k1[0:53, :], x1hi[1][:])
    ones2f2 = pk2op.tile([2, N], dt.float32, name="ones2f2")
    nc.vector.memset(ones2f2[:], 1.0)
    ones2n = pk2op.tile([2, N], dt.float32r, name="ones2n")
    nc.vector.tensor_copy(ones2n[:], ones2f2[:])
    nc.sync.dma_start(l2h_k1[53:55, :], ones2n[:])
    r2h_k0 = pk2op.tile([P, N], dt.float32r, name="r2h_k0")     # 2*x1hi[0]
    nc.vector.tensor_scalar_mul(r2h_k0[:], x1hi[0][:], 2.0)
    nc.vector.tensor_scalar_mul(r2h_k1[0:53, :], x1hi[1][:], 2.0)
    r2l0 = pk2op.tile([P, N], dt.float32r, name="r2l0")         # 2*x1lo[0]
    nc.vector.tensor_scalar_mul(r2l0[:], x1lo[0][:], 2.0)
    r2l1 = pk2op.tile([53, N], dt.float32r, name="r2l1")        # 2*x1lo[1]
    nc.vector.tensor_scalar_mul(r2l1[:], x1lo[1][:], 2.0)

    def mm2(pe, t, c):
        msl = slice(128 * t, 128 * (t + 1))
        csl = slice(512 * c, 512 * (c + 1))
        nc.tensor.matmul(pe[:, csl], x1hi[0][:, msl], r2h_k0[:, csl], start=True, stop=False)
        nc.tensor.matmul(pe[:, csl], l2h_k1[:, msl], r2h_k1[:, csl], start=False, stop=False)
        nc.tensor.matmul(pe[:, csl], x1hi[0][:, msl], r2l0[:, csl], start=False, stop=False)
        nc.tensor.matmul(pe[:, csl], x1hi[1][:, msl], r2l1[:, csl], start=False, stop=False)
        nc.tensor.matmul(pe[:, csl], x1lo[0][:, msl], r2h_k0[:, csl], start=False, stop=False)
        nc.tensor.matmul(pe[:, csl], x1lo[1][:, msl], r2h_k1[0:53, csl], start=False, stop=True)

    knn_pass(mm2, idx2_d, "k2")
    iw2 = build_wrapped(idx2_d, "k2")
    pk2op.release()
    k2pool.release()
    ph1.release()

    # ---------------- block 2 ----------------
    # conv2b weights early so their DMA overlaps b2a / AR3
    b2bpool = tc.alloc_tile_pool(name="b2bpool", bufs=1)
    w2bT_r, w2bT_b = [], []
    for i, rows in enumerate((128, 128, 128, 46)):
        wt = wld.tile([128, 595], dt.float32, name="wf")
        nc.sync.dma_start(wt[:rows, :512], ins[f"w2bT_k{i}"])
        wr = b2bpool.tile([rows, 512], dt.float32r, name=f"w2bT_r{i}")
        nc.vector.tensor_copy(wr[:], wt[:rows, :512])
        w2bT_r.append(wr)
        wb = b2bpool.tile([rows, 512], dt.bfloat16, name=f"w2bT_b{i}")
        nc.vector.tensor_copy(wb[:], wt[:rows, :512])
        w2bT_b.append(wb)

    # U2[c, n] = sum_ci du2[ci, c] x1[ci, n]   (plain f32r)
    b2pool = tc.alloc_tile_pool(name="b2pool", bufs=1)
    w2aT_r, du2T_r = [], []
    for i, rows in enumerate((128, 53)):
        wt = wld.tile([128, 595], dt.float32, name="wf")
        nc.sync.dma_start(wt[:rows, :430], ins[f"w2aT_k{i}"])
        wr = b2pool.tile([rows, 430], dt.float32r, name=f"w2aT_r{i}")
        nc.vector.tensor_copy(wr[:], wt[:rows, :430])
        w2aT_r.append(wr)
        ddt = wld.tile([128, 595], dt.float32, name="wf")
        nc.sync.dma_start(ddt[:rows, :430], ins[f"du2T_k{i}"])
        dr = b2pool.tile([rows, 430], dt.float32r, name=f"du2T_r{i}")
        nc.vector.tensor_copy(dr[:], ddt[:rows, :430])
        du2T_r.append(dr)
    rows2a_ = (128, 128, 128, 46)
    u2 = [b2pool.tile([r, N], dt.float32, name=f"u2_{i}") for i, r in enumerate(rows2a_)]
    pu2 = tc.alloc_tile_pool(name="pu2", bufs=4, space="PSUM")
    for m, rows in enumerate(rows2a_):
        msl = slice(128 * m, 128 * m + rows)
        for cc in range(4):
            sl = slice(512 * cc, 512 * (cc + 1))
            up = pu2.tile([rows, 512], dt.float32, name="up2", space="PSUM")
            nc.tensor.matmul(up[:], du2T_r[0][:, msl], x1hi[0][:, sl], start=True, stop=False)
            nc.tensor.matmul(up[:], du2T_r[1][:, msl], x1hi[1][:, sl], start=False, stop=True)
            nc.scalar.copy(u2[m][:, sl], up[:])
    pu2.release()

    rows2a = (128, 128, 128, 46)
    y2a_d = [dram.tile([r, S], dt.bfloat16, name=f"y2a_d{i}") for i, r in enumerate(rows2a)]
    sum2a = [b2pool.tile([r, NCHUNK], dt.float32, name=f"s2a_{i}") for i, r in enumerate(rows2a)]
    sq2a = [b2pool.tile([r, NCHUNK], dt.float32, name=f"q2a_{i}") for i, r in enumerate(rows2a)]

    # gather sources: bf16 copies of x1 (bf16 conv2a avoids f32r rounding casts)
    gx0 = b2pool.tile([P, N], dt.bfloat16, name="gx0")
    nc.vector.tensor_copy(gx0[:], x1hi[0][:].bitcast(dt.float32))
    gx1 = b2pool.tile([P, N], dt.bfloat16, name="gx1")
    nc.vector.tensor_copy(gx1[:], gx0[:])
    nc.vector.tensor_copy(gx1[0:53, :], x1hi[1][:].bitcast(dt.float32))
    gxi = b2pool.tile([P, 2 * N], dt.bfloat16, name="gxi")
    _g3 = gxi[:].rearrange("p (n two) -> p n two", two=2)
    nc.vector.tensor_copy(_g3[:, :, 0], gx0[:])
    nc.vector.tensor_copy(_g3[:, :, 1], gx1[:])
    w2aT_b = []
    for i, rows in enumerate((128, 53)):
        wb = b2pool.tile([rows, 430], dt.bfloat16, name=f"w2aT_b{i}")
        nc.vector.tensor_copy(wb[:], w2aT_r[i][:].bitcast(dt.float32))
        w2aT_b.append(wb)

    b2p = tc.alloc_tile_pool(name="b2p", bufs=2, space="PSUM")
    b2s = tc.alloc_tile_pool(name="b2s", bufs=4)
    b2o = tc.alloc_tile_pool(name="b2o", bufs=4)
    gxi3 = gxi[:].rearrange("p (n two) -> p n two", two=2)
    iw2d = b2pool.tile([P, S // 16], dt.uint16, name="iw2d")
    nc.vector.tensor_scalar_mul(iw2d[:], iw2[:], 2.0)
    b2q = {}
    def b2_gather(c):
        nbi = b2s.tile([P, 2 * CB], dt.bfloat16, name="nbi")
        nbi3 = nbi[:].rearrange("p (n two) -> p n two", two=2)
        for g in range(4):
            nc.gpsimd.indirect_copy(
                out=nbi3[:, 320 * g:320 * (g + 1), :], data=gxi3,
                idxs=iw2d[:, 80 * c + 20 * g:80 * c + 20 * (g + 1)],
                i_know_ap_gather_is_preferred=True)
        nb0r = b2s.tile([P, CB], dt.bfloat16, name="nb0r")
        nc.vector.tensor_copy(nb0r[:], nbi3[:, :, 0])
        nb1r = b2s.tile([P, CB], dt.bfloat16, name="nb1r")
        nc.vector.tensor_copy(nb1r[:], nbi3[:, :, 1])
        b2q[c] = (nb0r, nb1r)

    b2_gather(0)
    b2_gather(1)
    for c in range(NCHUNK):
        if c + 2 < NCHUNK:
            b2_gather(c + 2)
        nb0r, nb1r = b2q.pop(c)
        for m, rows in enumerate(rows2a):
            msl = slice(128 * m, 128 * m + rows)
            yp = b2p.tile([P, CB], dt.float32, name="y2ap", space="PSUM")
            for c0, c1 in CSL:
                nc.tensor.matmul(yp[:rows, c0:c1], w2aT_b[0][:, msl],
                                 nb0r[:, c0:c1], start=True, stop=False)
                nc.tensor.matmul(yp[:rows, c0:c1], w2aT_b[1][:, msl],
                                 nb1r[0:53, c0:c1], start=False, stop=True)
            uview = u2[m][:, 128 * c:128 * (c + 1), None].to_broadcast(
                [rows, 128, KNN])
            ob = b2o.tile([rows, CB], dt.bfloat16, name="ob2")
            nc.vector.scalar_tensor_tensor(
                out=ob[:].rearrange("p (n k) -> p n k", k=KNN),
                in0=yp[:rows, :].rearrange("p (n k) -> p n k", k=KNN), scalar=1.0,
                in1=uview, op0=ALU.mult, op1=ALU.add,
                accum_out=sum2a[m][:, c:c + 1])
            nc.sync.dma_start(y2a_d[m][:, CB * c:CB * (c + 1)], ob[:])
            tr = b1t.tile([rows, CB], dt.bfloat16, name="tr")
            nc.scalar.activation(tr[:], ob[:], AF.Square,
                                 accum_out=sq2a[m][:, c:c + 1])
    b2p.release()
    b2o.release()
    b2s.release()

    s2 = [_acc_reduce(nc, per, sum2a[m][:], rows2a[m], f"s2r{m}") for m in range(4)]
    q2 = [_acc_reduce(nc, per, sq2a[m][:], rows2a[m], f"q2r{m}") for m in range(4)]
    b2pool.release()

    # block-3 weights early (LIFO: pw3 below b3p/b3l/c3t/px2r); DMA overlaps conv2b
    pw3 = tc.alloc_tile_pool(name="pw3", bufs=1)
    w3aT_r = []
    rows3a_k = (128, 53, 128, 128, 128, 128)
    for i, rows in enumerate(rows3a_k):
        wt = wld.tile([128, 595], dt.float32, name="wf")
        nc.sync.dma_start(wt[:rows, :595], ins[f"w3aT_k{i}"])
        wr = pw3.tile([rows, 595], dt.bfloat16, name=f"w3aT_r{i}")
        nc.vector.tensor_copy(wr[:], wt[:rows, :595])
        w3aT_r.append(wr)
    w3bT_r = []
    rows3b_k = (128, 128, 128, 128, 83)
    for i, rows in enumerate(rows3b_k):
        wt = wld.tile([128, 595], dt.float32, name="wf")
        nc.sync.dma_start(wt[:rows, :512], ins[f"w3bT_k{i}"])
        wr = pw3.tile([rows, 512], dt.bfloat16, name=f"w3bT_r{i}")
        nc.vector.tensor_copy(wr[:], wt[:rows, :512])
        w3bT_r.append(wr)
    pb3 = tc.alloc_tile_pool(name="pb3", bufs=1)   # y3a/y3b bf16 residency

    rows2b = (128, 128, 128, 128)
    sumh2 = [b2bpool.tile([r, NCHUNK], dt.float32, name=f"sh2_{i}") for i, r in enumerate(rows2a)]
    sq2b = [b2bpool.tile([r, NCHUNK], dt.float32, name=f"q2b_{i}") for i, r in enumerate(rows2b)]
    m2 = [b2bpool.tile([r, N], dt.bfloat16, name=f"m2_{i}") for i, r in enumerate(rows2b)]

    b3p = tc.alloc_tile_pool(name="b3p", bufs=2, space="PSUM")
    b3l = tc.alloc_tile_pool(name="b3l", bufs=3)

    c2q = {}
    def c2_load(c):
        csl = slice(CB * c, CB * (c + 1))
        grp = []
        for m, rows in enumerate(rows2a):
            ld = b3l.tile([rows, CB], dt.bfloat16, name=f"ld{m}")
            nc.sync.dma_start(ld[:], y2a_d[m][:, csl])
            grp.append(ld)
        c2q[c] = grp

    c2_load(0)
    ar3 = _allreduce(nc, dram, per, [s[:] for s in s2], [q[:] for q in q2],
                     rows2a, 430, "ar3")
    sb2a = [_bn_coeffs(nc, per, ar3[m][0][:], ar3[m][1][:], g2a[m][:], b2a[m][:],
                       rows2a[m], CNT, f"bn2a{m}") for m in range(4)]
    for c in range(NCHUNK):
        if c + 1 < NCHUNK:
            c2_load(c + 1)
        csl = slice(CB * c, CB * (c + 1))
        h2 = c2q.pop(c)
        for m, rows in enumerate(rows2a):
            ld = h2[m]
            nc.scalar.activation(ld[:], ld[:], AF.Prelu, bias=sb2a[m][1][:],
                                 scale=sb2a[m][0][:], alpha=0.2,
                                 accum_out=sumh2[m][:, c:c + 1])
        for m, rows in enumerate(rows2b):
            msl = slice(128 * m, 128 * (m + 1))
            yp = b3p.tile([P, CB], dt.float32, name="y2bp", space="PSUM")
            for c0, c1 in CSL:
                for k in range(4):
                    kr = (128, 128, 128, 46)[k]
                    nc.tensor.matmul(yp[:rows, c0:c1], w2bT_b[k][:, msl],
                                     h2[k][0:kr, c0:c1],
                                     start=(k == 0), stop=(k == 3))
            tr = b1t.tile([rows, CB], dt.bfloat16, name="tr")
            nc.scalar.activation(tr[:], yp[:rows, :], AF.Square,
                                 accum_out=sq2b[m][:, c:c + 1])
            nc.vector.tensor_reduce(out=m2[m][:, 128 * c:128 * (c + 1)],
                                    in_=yp[:rows, :].rearrange("p (n k) -> p n k", k=KNN),
                                    axis=AX.X, op=ALU.max)
    b3p.release()
    b3l.release()

    sh2 = [_acc_reduce(nc, per, sumh2[m][:], rows2a[m], f"sh2r{m}") for m in range(4)]
    sh2r = []
    for m in range(4):
        t = per.tile([rows2a[m], 4], dt.float32r, name=f"sh2rr{m}")
        nc.vector.tensor_copy(t[:], sh2[m][:].to_broadcast([rows2a[m], 4]))
        sh2r.append(t)
    ps2b = tc.alloc_tile_pool(name="ps2b", bufs=1, space="PSUM")
    s2b = []
    for m in range(4):
        msl = slice(128 * m, 128 * (m + 1))
        sp = ps2b.tile([128, 4], dt.float32, name=f"s2bp{m}", space="PSUM")
        for k in range(4):
            kr = (128, 128, 128, 46)[k]
            nc.tensor.matmul(sp[:], w2bT_r[k][:, msl], sh2r[k][0:kr, :],
                             start=(k == 0), stop=(k == 3))
        st = per.tile([128, 1], dt.float32, name=f"s2b_{m}")
        nc.scalar.copy(st[:], sp[:, 0:1])
        s2b.append(st)
    ps2b.release()
    q2b = [_acc_reduce(nc, per, sq2b[m][:], 128, f"q2br{m}") for m in range(4)]
    ar4 = _allreduce(nc, dram, per, [s[:] for s in s2b], [q[:] for q in q2b],
                     rows2b, 512, "ar4")
    sb2b = [_bn_coeffs(nc, per, ar4[m][0][:], ar4[m][1][:], g2b[m][:], b2b[m][:],
                       128, CNT, f"bn2b{m}") for m in range(4)]

    # x2 = prelu(bn(m2)) -> bf16; x1 recast to bf16 for block 3
    c3t = tc.alloc_tile_pool(name="c3t", bufs=4)
    px2r = tc.alloc_tile_pool(name="px2r", bufs=1)
    x2r = [px2r.tile([128, N], dt.bfloat16, name=f"x2r_{i}") for i in range(4)]
    for m in range(4):
        nc.scalar.activation(x2r[m][:], m2[m][:], AF.Prelu, bias=sb2b[m][1][:],
                             scale=sb2b[m][0][:], alpha=0.2)
    xb1 = [px2r.tile([r, N], dt.bfloat16, name=f"xb1_{i}")
           for i, r in enumerate((128, 53))]
    for i in range(2):
        nc.vector.tensor_copy(xb1[i][:], x1hi[i][:].bitcast(dt.float32))

    # ---------------- block 3 ----------------
    # y3a / y3b stay SBUF-resident in bf16 (post-x1 path tolerates bf16)
    xc_k = [xb1[0], xb1[1], x2r[0], x2r[1], x2r[2], x2r[3]]
    rows3a = (128, 128, 128, 128, 83)
    y3a_s = [pb3.tile([r, N], dt.bfloat16, name=f"y3a_s{i}") for i, r in enumerate(rows3a)]
    sum3a = [per.tile([r, 4], dt.float32, name=f"s3a_{i}") for i, r in enumerate(rows3a)]
    sq3a = [per.tile([r, 4], dt.float32, name=f"q3a_{i}") for i, r in enumerate(rows3a)]
    c3p = tc.alloc_tile_pool(name="c3p", bufs=4, space="PSUM")
    for c in range(4):
        csl = slice(512 * c, 512 * (c + 1))
        for m, rows in enumerate(rows3a):
            msl = slice(128 * m, 128 * m + rows)
            yp = c3p.tile([128, 512], dt.float32, name="y3ap", space="PSUM")
            for k in range(6):
                nc.tensor.matmul(yp[:rows, :], w3aT_r[k][:, msl], xc_k[k][:, csl],
                                 start=(k == 0), stop=(k == 5))
            nc.scalar.activation(y3a_s[m][:, csl], yp[:rows, :], AF.Copy,
                                 accum_out=sum3a[m][:, c:c + 1])
            tr = c3t.tile([rows, 512], dt.bfloat16, name="tr")
            nc.scalar.activation(tr[:], yp[:rows, :], AF.Square,
                                 accum_out=sq3a[m][:, c:c + 1])
    c3p.release()
    px2r.release()

    s3 = [_acc_reduce(nc, per, sum3a[m][:], rows3a[m], f"s3r{m}") for m in range(5)]
    q3 = [_acc_reduce(nc, per, sq3a[m][:], rows3a[m], f"q3r{m}") for m in range(5)]
    ar5 = _allreduce(nc, dram, per, [s[:] for s in s3], [q[:] for q in q3],
                     rows3a, 640, "ar5")
    sb3a = [_bn_coeffs(nc, per, ar5[m][0][:], ar5[m][1][:], g3a[m][:], b3a[m][:],
                       rows3a[m], CNT3, f"bn3a{m}") for m in range(5)]

    rows3b = (128, 128, 128, 128)
    y3b_s = [pb3.tile([r, N], dt.bfloat16, name=f"y3b_s{i}") for i, r in enumerate(rows3b)]
    sum3b = [per.tile([r, 4], dt.float32, name=f"s3b_{i}") for i, r in enumerate(rows3b)]
    sq3b = [per.tile([r, 4], dt.float32, name=f"q3b_{i}") for i, r in enumerate(rows3b)]
    c4p = tc.alloc_tile_pool(name="c4p", bufs=2, space="PSUM")
    h3l = tc.alloc_tile_pool(name="h3l", bufs=10)
    for c in range(4):
        csl = slice(512 * c, 512 * (c + 1))
        h3c = []
        for k, kr in enumerate(rows3a):
            ld = h3l.tile([kr, 512], dt.bfloat16, name="h3ld")
            nc.scalar.activation(ld[:], y3a_s[k][:, csl], AF.Prelu,
                                 bias=sb3a[k][1][:], scale=sb3a[k][0][:], alpha=0.2)
            h3c.append(ld)
        for m, rows in enumerate(rows3b):
            msl = slice(128 * m, 128 * (m + 1))
            yp = c4p.tile([rows, 512], dt.float32, name=f"y3bp{m}", space="PSUM")
            for k in range(5):
                nc.tensor.matmul(yp[:], w3bT_r[k][:, msl], h3c[k][:],
                                 start=(k == 0), stop=(k == 4))
            nc.scalar.activation(y3b_s[m][:, csl], yp[:], AF.Copy,
                                 accum_out=sum3b[m][:, c:c + 1])
            tr = c3t.tile([rows, 512], dt.bfloat16, name="tr")
            nc.scalar.activation(tr[:], yp[:], AF.Square,
                                 accum_out=sq3b[m][:, c:c + 1])
    c4p.release()
    h3l.release()
    c3t.release()

    s4 = [_acc_reduce(nc, per, sum3b[m][:], 128, f"s4r{m}") for m in range(4)]
    q4 = [_acc_reduce(nc, per, sq3b[m][:], 128, f"q4r{m}") for m in range(4)]
    ar6 = _allreduce(nc, dram, per, [s[:] for s in s4], [q[:] for q in q4],
                     rows3b, 512, "ar6")
    sb3b = [_bn_coeffs(nc, per, ar6[m][0][:], ar6[m][1][:], g3b[m][:], b3b[m][:],
                       128, CNT3, f"bn3b{m}") for m in range(4)]

    out_d = nc._ext_outputs["out"]
    for m in range(4):
        for c in range(4):
            csl = slice(512 * c, 512 * (c + 1))
            ot = oseg.tile([128, 512], dt.float32, name="ot")
            nc.scalar.activation(ot[:], y3b_s[m][:, csl], AF.Prelu,
                                 bias=sb3b[m][1][:], scale=sb3b[m][0][:], alpha=0.2)
            nc.sync.dma_start(out_d[128 * m:128 * (m + 1), csl], ot[:])
    pb3.release()
    pw3.release()
    b2bpool.release()
    wld.release()
    oseg.release()
    b1t.release()
    per.release()


# ------------------------------------------------------------------ host side
_CACHE = {}


def _build():
    _install_bassfix()
    nc = bass.Bass("TRN2", target_bir_lowering=False, debug=False,
                   num_devices=NCORES)
    in_specs = {
        "x": (64, N), "w1aT": (64, 152), "du1T": (64, 152),
        "w1bT_k0": (128, 181), "w1bT_k1": (24, 181),
        "w2aT_k0": (128, 430), "w2aT_k1": (53, 430),
        "du2T_k0": (128, 430), "du2T_k1": (53, 430),
        "w2bT_k0": (128, 512), "w2bT_k1": (128, 512),
        "w2bT_k2": (128, 512), "w2bT_k3": (46, 512),
        "w3aT_k0": (128, 595), "w3aT_k1": (53, 595), "w3aT_k2": (128, 595),
        "w3aT_k3": (128, 595), "w3aT_k4": (128, 595), "w3aT_k5": (128, 595),
        "w3bT_k0": (128, 512), "w3bT_k1": (128, 512), "w3bT_k2": (128, 512),
        "w3bT_k3": (128, 512), "w3bT_k4": (83, 512),
        "g1a": (152, 1), "be1a": (152, 1), "g1b": (181, 1), "be1b": (181, 1),
        "g2a": (430, 1), "be2a": (430, 1), "g2b": (512, 1), "be2b": (512, 1),
        "g3a": (595, 1), "be3a": (595, 1), "g3b": (512, 1), "be3b": (512, 1),
        "E": (32, CB),
    }
    nc._ext_inputs = {}
    for nm, shp in in_specs.items():
        nc._ext_inputs[nm] = nc.dram_tensor(nm, list(shp), dt.float32,
                                            kind="ExternalInput").ap()
    nc._ext_outputs = {
        "out": nc.dram_tensor("out", [512, N], dt.float32,
                              kind="ExternalOutput").ap()}
    with tile.TileContext(nc) as tc:
        build_kernel(nc, tc)
    return nc


def _host_inputs(x, c1w1, c1g1, c1be1, c1w2, c1g2, c1be2,
                 c2w1, c2g1, c2be1, c2w2, c2g2, c2be2,
                 c3w1, c3g1, c3be1, c3w2, c3g2, c3be2):
    f32 = np.float32
    W1 = np.asarray(c1w1, f32)            # [152, 128]
    W1a, W1b = W1[:, :64], W1[:, 64:]
    W2 = np.asarray(c2w1, f32)            # [430, 362]
    W2a, W2b = W2[:, :181], W2[:, 181:]
    ws = {
        "w1aT": np.ascontiguousarray(W1a.T),
        "du1T": np.ascontiguousarray((W1b - W1a).T),
    }
    w1bT = np.ascontiguousarray(np.asarray(c1w2, f32).T)     # [152, 181]
    ws["w1bT_k0"], ws["w1bT_k1"] = w1bT[:128], w1bT[128:]
    w2aT = np.ascontiguousarray(W2a.T)                        # [181, 430]
    du2T = np.ascontiguousarray((W2b - W2a).T)
    ws["w2aT_k0"], ws["w2aT_k1"] = w2aT[:128], w2aT[128:]
    ws["du2T_k0"], ws["du2T_k1"] = du2T[:128], du2T[128:]
    w2bT = np.ascontiguousarray(np.asarray(c2w2, f32).T)     # [430, 512]
    for i, sl in enumerate((slice(0, 128), slice(128, 256), slice(256, 384),
                            slice(384, 430))):
        ws[f"w2bT_k{i}"] = w2bT[sl]
    w3aT = np.ascontiguousarray(np.asarray(c3w1, f32).T)     # [693, 595]
    cuts = (0, 128, 181, 309, 437, 565, 693)
    for i in range(6):
        ws[f"w3aT_k{i}"] = w3aT[cuts[i]:cuts[i + 1]]
    w3bT = np.ascontiguousarray(np.asarray(c3w2, f32).T)     # [595, 512]
    for i in range(5):
        ws[f"w3bT_k{i}"] = w3bT[128 * i:min(128 * (i + 1), 595)]
    for nm, v in (("g1a", c1g1), ("be1a", c1be1), ("g1b", c1g2), ("be1b", c1be2),
                  ("g2a", c2g1), ("be2a", c2be1), ("g2b", c2g2), ("be2b", c2be2),
                  ("g3a", c3g1), ("be3a", c3be1), ("g3b", c3g2), ("be3b", c3be2)):
        ws[nm] = np.ascontiguousarray(np.asarray(v, f32).reshape(-1, 1))
    E = np.zeros((32, CB), f32)
    for r in range(32):
        E[r, KNN * r:KNN * (r + 1)] = 1.0
    ws["E"] = E
    ws = {k: np.ascontiguousarray(v, f32) for k, v in ws.items()}
    xs = np.asarray(x, f32)
    in_maps = []
    for i in range(NCORES):
        m = dict(ws)
        m["x"] = np.ascontiguousarray(xs[i])
        in_maps.append(m)
    return in_maps


def kernel(x, c1w1, c1b1, c1g1, c1be1, c1w2, c1b2, c1g2, c1be2,
           c2w1, c2b1, c2g1, c2be1, c2w2, c2b2, c2g2, c2be2,
           c3w1, c3b1, c3g1, c3be1, c3w2, c3b2, c3g2, c3be2):
    # conv biases are absorbed exactly by the following BatchNorm (shift
    # invariance), so b* inputs are unused by the device program.
    if "nc" not in _CACHE:
        _CACHE["nc"] = _build()
    nc = _CACHE["nc"]
    in_maps = _host_inputs(x, c1w1, c1g1, c1be1, c1w2, c1g2, c1be2,
                           c2w1, c2g1, c2be1, c2w2, c2g2, c2be2,
                           c3w1, c3g1, c3be1, c3w2, c3g2, c3be2)
    res = run_bass_kernel_spmd(nc, in_maps, core_ids=list(range(NCORES)))
    _CACHE["last_result"] = res
    out = np.stack([res.results[i]["out"] for i in range(NCORES)], axis=0)
    return out.astype(np.float32)



# revision 33
# speedup vs baseline: 1.0890x; 1.0890x over previous
"""DGCNN-style GCN encoder on 8 TRN2 NeuronCores (bass/tile).

Data-parallel over batch: each core owns one sample (B=8). BatchNorm batch
statistics are all-reduced across the 8 cores so results match global-batch
BN exactly. kNN top-10, neighbor gathers, and all convs/BN/activations run
on-device; the host only reshapes/replicates weights.

Precision strategy: matmuls on the x -> x1 -> knn2 path use a compensated
fp32r (hi/lo split) scheme that recovers ~fp32 accuracy at fp32r speed
(conv1b uses plain fp32). This keeps the kNN neighbor selection stable
against the f32 reference. Everything downstream of x1 (conv2a/2b, block 3)
runs in bf16 (weights + activations), which the 2e-2 tolerance absorbs.

Performance notes (vs the first working version):
- edge loops process 1280-edge chunks (128 points x 10 nbrs) instead of 320
- neighbor gathers are indirect_copy, whose cost is ~28ns per index of Q7
  dispatch; conv2a gathers both x1 row-tiles in ONE pass by interleaving
  them as bf16 pairs (d=2 gather with doubled indices), halving index count
- gathers/loads are software-pipelined one-to-two chunks ahead, and the
  next phase's first loads + weight loads are issued before each BN
  AllReduce so DMA overlaps the collective
- y3a/y3b stay SBUF-resident in bf16 (no DRAM round-trip); y2a is spilled
  to DRAM in bf16 (half the traffic)
"""

import os
import sys
import json

B16 = "/nix/store/wxap7svlj45h0lfm31d1axjjnzyl6qsy-b16-bazel-unstable-cc-2026-05-04-9a3fa1f3-rt-2026-05-04-ade39e0a/lib/python3.13/site-packages"
if B16 not in sys.path:
    sys.path.insert(0, B16)
if "/opt/trn_rl_repo" not in sys.path:
    sys.path.insert(0, "/opt/trn_rl_repo")

import numpy as np
import concourse.bass as bass
import concourse.mybir as mybir
import concourse.tile as tile
from concourse.bass_utils import run_bass_kernel_spmd

dt = mybir.dt
AF = mybir.ActivationFunctionType
ALU = mybir.AluOpType
AX = mybir.AxisListType

N = 2048
KNN = 10
S = N * KNN            # 20480 edge positions
CB = 1280              # banded chunk: 128 points x 10 neighbors
NCHUNK = S // CB       # 16
CSL = [(0, 512), (512, 1024), (1024, 1280)]   # matmul col slices of a chunk
EPS = 1e-5
NCORES = 8
CNT = float(NCORES * S)      # BN count for edge tensors
CNT3 = float(NCORES * N)     # BN count for block-3 tensors


# ---------------------------------------------------------------- walrus fix
def _fix_bir_json(bir_json: bytes) -> bytes:
    """This container's walrus accepts only ONE sync-wait per instruction;
    hoist extra waits onto preceding EventSemaphore instructions."""
    m = json.loads(bir_json)
    for f in m["functions"]:
        for bb in f["blocks"]:
            out = []
            for i in bb["instructions"]:
                si = i.get("sync_info") or {}
                waits = si.get("on_wait") or []
                if len(waits) > 1:
                    for k, w in enumerate(waits[:-1]):
                        out.append({
                            "name": f"{i['name']}w{k}",
                            "engine": i["engine"],
                            "opcode": "EventSemaphore",
                            "ins": [], "outs": [],
                            "debug": i.get("debug"),
                            "sync_info": {"on_update": [], "on_wait": [w]},
                        })
                    i = dict(i)
                    si = dict(si)
                    si["on_wait"] = [waits[-1]]
                    i["sync_info"] = si
                out.append(i)
            bb["instructions"] = out
    return json.dumps(m).encode()


def _install_bassfix():
    import concourse.bass_utils as bu
    import concourse.bass2jax as b2j
    if getattr(bu, "_bassfix_installed", False):
        return
    real = bu.compile_bir_kernel

    def patched(bir_json, tmpdir, neff_name="file.neff"):
        return real(_fix_bir_json(bir_json), tmpdir, neff_name)

    bu.compile_bir_kernel = patched
    b2j.compile_bir_kernel = patched
    bu._bassfix_installed = True


# ------------------------------------------------------------- device helpers
def _round_split(nc, pool, src_ap, parts, width, name):
    """Return (hi, lo) float32r tiles for a f32 source AP [parts, width]."""
    hi = pool.tile([parts, width], dt.float32r, name=name + "_hi")
    lo = pool.tile([parts, width], dt.float32r, name=name + "_lo")
    nc.vector.tensor_copy(hi[:], src_ap)
    nc.vector.tensor_tensor(out=lo[:], in0=src_ap, in1=hi[:].bitcast(dt.float32),
                            op=ALU.subtract)
    return hi, lo


def _bn_coeffs(nc, pool, sums, sqs, g_ap, b_ap, rows, cnt, name):
    """sums/sqs: [rows,1] f32 (global). Returns (scale, bias) [rows,1] f32."""
    mean = pool.tile([rows, 1], dt.float32, name=name + "_mean")
    nc.vector.tensor_scalar_mul(mean[:], sums, 1.0 / cnt)
    e2 = pool.tile([rows, 1], dt.float32, name=name + "_e2")
    nc.vector.tensor_scalar_mul(e2[:], sqs, 1.0 / cnt)
    m2 = pool.tile([rows, 1], dt.float32, name=name + "_m2")
    nc.vector.tensor_tensor(out=m2[:], in0=mean[:], in1=mean[:], op=ALU.mult)
    var = pool.tile([rows, 1], dt.float32, name=name + "_var")
    nc.vector.tensor_tensor(out=var[:], in0=e2[:], in1=m2[:], op=ALU.subtract)
    epst = pool.tile([rows, 1], dt.float32, name=name + "_eps")
    nc.vector.memset(epst[:], float(EPS))
    std = pool.tile([rows, 1], dt.float32, name=name + "_std")
    nc.scalar.activation(std[:], var[:], AF.Sqrt, bias=epst[:])
    rstd = pool.tile([rows, 1], dt.float32, name=name + "_rstd")
    nc.vector.reciprocal(rstd[:], std[:])
    scale = pool.tile([rows, 1], dt.float32, name=name + "_scale")
    nc.vector.tensor_tensor(out=scale[:], in0=rstd[:], in1=g_ap, op=ALU.mult)
    ms = pool.tile([rows, 1], dt.float32, name=name + "_ms")
    nc.vector.tensor_tensor(out=ms[:], in0=mean[:], in1=scale[:], op=ALU.mult)
    bias = pool.tile([rows, 1], dt.float32, name=name + "_bias")
    nc.vector.tensor_tensor(out=bias[:], in0=b_ap, in1=ms[:], op=ALU.subtract)
    return scale, bias


def _allreduce(nc, dram, per, sums_list, sqs_list, rows_list, cpad, name):
    """Pack per-Mtile [rows,1] sum/sq into a DRAM [cpad,2] buffer, AllReduce
    across the 8 cores, unpack back into fresh [rows,1] tiles."""
    cin = dram.tile([cpad, 2], dt.float32, name=name + "_in")
    cout = dram.tile([cpad, 2], dt.float32, name=name + "_out")
    r0 = 0
    for s_, q_, rows in zip(sums_list, sqs_list, rows_list):
        nc.sync.dma_start(cin[r0:r0 + rows, 0:1], s_)
        nc.sync.dma_start(cin[r0:r0 + rows, 1:2], q_)
        r0 += rows
    if r0 < cpad:
        z = per.tile([cpad - r0, 2], dt.float32, name=name + "_z")
        nc.vector.memset(z[:], 0.0)
        nc.sync.dma_start(cin[r0:cpad, :], z[:])
    nc.gpsimd.collective_compute(
        "AllReduce", ALU.add, replica_groups=[list(range(NCORES))],
        ins=[cin[:].opt()], outs=[cout[:].opt()])
    outs = []
    r0 = 0
    for rows in rows_list:
        gs = per.tile([rows, 1], dt.float32, name=name + f"_gs{r0}")
        gq = per.tile([rows, 1], dt.float32, name=name + f"_gq{r0}")
        nc.sync.dma_start(gs[:], cout[r0:r0 + rows, 0:1])
        nc.sync.dma_start(gq[:], cout[r0:r0 + rows, 1:2])
        outs.append((gs, gq))
        r0 += rows
    return outs


def _acc_reduce(nc, per, acc, rows, name):
    """[rows, NCHUNK] accumulator -> [rows,1]."""
    out = per.tile([rows, 1], dt.float32, name=name)
    nc.vector.tensor_reduce(out=out[:], in_=acc, axis=AX.X, op=ALU.add)
    return out


def build_kernel(nc, tc):
    P = 128
    per = tc.alloc_tile_pool(name="persist", bufs=1)
    dram = tc.alloc_tile_pool(name="dram", bufs=1, space="DRAM")
    b1t = tc.alloc_tile_pool(name="b1t", bufs=2)
    oseg = tc.alloc_tile_pool(name="oseg", bufs=1)
    wld = tc.alloc_tile_pool(name="wld", bufs=2)
    ph1 = tc.alloc_tile_pool(name="ph1", bufs=1)

    ins = nc._ext_inputs
    x_d = ins["x"]

    # ---------------- load & prep ----------------
    tmp0 = tc.alloc_tile_pool(name="tmp0", bufs=1)      # until U1t/rhs built
    x = tmp0.tile([64, N], dt.float32, name="x")
    nc.sync.dma_start(x[:], x_d)
    xhi, xlo = _round_split(nc, tmp0, x[:], 64, N, "x")
    # stacks for compensated matmuls
    xstack = ph1.tile([P, N], dt.float32r, name="xstack")   # [xhi; xlo]
    nc.vector.tensor_copy(xstack[:64, :], xhi[:])
    nc.vector.tensor_copy(xstack[64:, :], xlo[:])
    lhsx = xstack                                             # [xhi; xlo]
    rhsx = ph1.tile([P, N], dt.float32r, name="rhsx")        # 2*[xlo; xhi]
    nc.vector.tensor_scalar_mul(rhsx[:64, :], xlo[:], 2.0)
    nc.vector.tensor_scalar_mul(rhsx[64:, :], xhi[:], 2.0)

    # sq1 = sum_c x^2 (compensated)
    xsq = tmp0.tile([64, N], dt.float32, name="xsq")
    nc.vector.tensor_tensor(out=xsq[:], in0=x[:], in1=x[:], op=ALU.mult)
    xsqhi, xsqlo = _round_split(nc, tmp0, xsq[:], 64, N, "xsq")
    ones_f = per.tile([128, 1], dt.float32, name="ones_f")
    nc.vector.memset(ones_f[:], 1.0)
    ones64 = per.tile([64, 1], dt.float32r, name="ones64")
    nc.vector.tensor_copy(ones64[:], ones_f[:64, :])
    # knn1 matmul operands (sq1 folded in chunk-wise)
    lhs_hi1 = ph1.tile([66, N], dt.float32r, name="lhs_hi1")
    nc.vector.tensor_copy(lhs_hi1[:64, :], xhi[:])
    ones2f = tmp0.tile([2, N], dt.float32, name="ones2f")
    nc.vector.memset(ones2f[:], 1.0)
    nc.vector.tensor_copy(lhs_hi1[64:66, :], ones2f[:])
    rhs_hi1 = ph1.tile([66, N], dt.float32r, name="rhs_hi1")
    nc.vector.tensor_scalar_mul(rhs_hi1[:64, :], xhi[:], 2.0)
    psq = tc.alloc_tile_pool(name="psq", bufs=1, space="PSUM")
    sq1p = psq.tile([1, N], dt.float32, name="sq1p", space="PSUM")
    for c in range(4):
        sl = slice(512 * c, 512 * (c + 1))
        nc.tensor.matmul(sq1p[:, sl], ones64[:], xsqhi[:, sl], start=True, stop=False)
        nc.tensor.matmul(sq1p[:, sl], ones64[:], xsqlo[:, sl], start=False, stop=True)
        sc_ = tmp0.tile([1, 512], dt.float32, name="sq1c")
        nc.scalar.copy(sc_[:], sq1p[:, sl])
        sh_ = tmp0.tile([1, 512], dt.float32r, name="sq1ch")
        nc.vector.tensor_copy(sh_[:], sc_[:])
        sl_ = tmp0.tile([1, 512], dt.float32r, name="sq1cl")
        nc.vector.tensor_tensor(out=sl_[:], in0=sc_[:], in1=sh_[:].bitcast(dt.float32),
                                op=ALU.subtract)
        nc.vector.tensor_scalar_mul(rhs_hi1[64:65, sl], sh_[:], -1.0)
        nl_ = tmp0.tile([1, 512], dt.float32r, name="sq1nl")
        nc.vector.tensor_scalar_mul(nl_[:], sl_[:], -1.0)
        nc.sync.dma_start(rhs_hi1[65:66, sl], nl_[:])
    psq.release()

    # conv1 weights
    w1aT = wld.tile([128, 595], dt.float32, name="wf")
    nc.sync.dma_start(w1aT[:64, :152], ins["w1aT"])
    w1aT_hi, w1aT_lo = _round_split(nc, per, w1aT[:64, :152], 64, 152, "w1aT")
    du1T = wld.tile([128, 595], dt.float32, name="wf")
    nc.sync.dma_start(du1T[:64, :152], ins["du1T"])
    du1T_hi, du1T_lo = _round_split(nc, per, du1T[:64, :152], 64, 152, "du1T")
    w1aX = per.tile([P, 152], dt.float32r, name="w1aX")      # [Wlo; Whi]
    nc.vector.tensor_copy(w1aX[:64, :], w1aT_lo[:])
    nc.vector.tensor_copy(w1aX[64:, :], w1aT_hi[:])
    du1X = per.tile([P, 152], dt.float32r, name="du1X")      # [dlo; dhi] pairs lhsx
    nc.vector.tensor_copy(du1X[:64, :], du1T_lo[:])
    nc.vector.tensor_copy(du1X[64:, :], du1T_hi[:])

    wkt1b = []  # conv1b fp32 weights, K-tiles
    for i, rows in enumerate((128, 24)):
        t = per.tile([rows, 181], dt.float32, name=f"w1bT_k{i}")
        nc.sync.dma_start(t[:], ins[f"w1bT_k{i}"])
        wkt1b.append(t)


    # U1[c, n] = sum_ci du1[ci, c] x[ci, n]   (compensated, natural layout)
    rows1a_ = (128, 24)
    u1 = [ph1.tile([r, N], dt.float32, name=f"u1_{i}") for i, r in enumerate(rows1a_)]
    pu = tc.alloc_tile_pool(name="pu", bufs=4, space="PSUM")
    for m, rows in enumerate(rows1a_):
        msl = slice(128 * m, 128 * m + rows)
        for cc in range(4):
            sl = slice(512 * cc, 512 * (cc + 1))
            up = pu.tile([rows, 512], dt.float32, name="up", space="PSUM")
            nc.tensor.matmul(up[:], du1T_hi[:, msl], xhi[:, sl], start=True, stop=False)
            nc.tensor.matmul(up[:], du1X[:, msl], xstack[:, sl], start=False, stop=True)
            nc.scalar.copy(u1[m][:, sl], up[:])
    pu.release()
    tmp0.release()

    # BN params
    def load_vec(nm, rows_list):
        tiles = []
        r0 = 0
        for i, rows in enumerate(rows_list):
            t = per.tile([rows, 1], dt.float32, name=f"{nm}_{i}")
            nc.sync.dma_start(t[:], ins[nm][r0:r0 + rows, :])
            tiles.append(t)
            r0 += rows
        return tiles

    g1a = load_vec("g1a", (128, 24)); b1a = load_vec("be1a", (128, 24))
    g1b = load_vec("g1b", (128, 53)); b1b = load_vec("be1b", (128, 53))
    g2a = load_vec("g2a", (128, 128, 128, 46)); b2a = load_vec("be2a", (128, 128, 128, 46))
    g2b = load_vec("g2b", (128,) * 4); b2b = load_vec("be2b", (128,) * 4)
    g3a = load_vec("g3a", (128,) * 4 + (83,)); b3a = load_vec("be3a", (128,) * 4 + (83,))
    g3b = load_vec("g3b", (128,) * 4); b3b = load_vec("be3b", (128,) * 4)

    # ---------------- kNN (shared machinery) ----------------
    idx1_d = dram.tile([16, P, KNN], dt.uint16, name="idx1_d")
    idx2_d = dram.tile([16, P, KNN], dt.uint16, name="idx2_d")

    def knn_pass(mm_emit, idx_dram, tag, after_tile=None, kp_bufs=2):
        """mm_emit(psum_slice, c) emits matmuls for columns 512c..512c+512."""
        kp = tc.alloc_tile_pool(name=f"knnp_{tag}", bufs=kp_bufs, space="PSUM")
        ks = tc.alloc_tile_pool(name=f"knns_{tag}", bufs=2)
        for t in range(16):
            pe = kp.tile([P, N], dt.float32, name="pe", space="PSUM")
            for c in range(4):
                mm_emit(pe, t, c)
            q = ks.tile([P, N], dt.float32, name="q")
            nc.scalar.copy(q[:], pe[:])
            v8 = ks.tile([P, 8], dt.float32, name="v8")
            i8 = ks.tile([P, 8], dt.uint16, name="i8")
            nc.vector.max(out=v8[:], in_=q[:])
            nc.vector.max_index(out=i8[:], in_max=v8[:], in_values=q[:])
            nc.vector.match_replace(out=q[:], in_to_replace=v8[:], in_values=q[:],
                                    imm_value=-1e30)
            v8b = ks.tile([P, 8], dt.float32, name="v8b")
            i8b = ks.tile([P, 8], dt.uint16, name="i8b")
            nc.vector.max(out=v8b[:], in_=q[:])
            nc.vector.max_index(out=i8b[:], in_max=v8b[:], in_values=q[:])
            idx = ks.tile([P, KNN], dt.uint16, name="idx")
            nc.vector.tensor_copy(idx[:, 0:7], i8[:, 1:8])
            nc.vector.tensor_copy(idx[:, 7:10], i8b[:, 0:3])
            nc.sync.dma_start(idx_dram[t], idx[:])
            if after_tile is not None:
                after_tile(t)
        kp.release()
        ks.release()

    def build_wrapped(idx_dram, tag):
        """DRAM flat idx (point-major) -> wrapped [128, S/16] uint16."""
        iw = per.tile([P, S // 16], dt.uint16, name=f"iw_{tag}")
        flat = idx_dram[:].rearrange("a p k -> (a p k)")
        nc.sync.dma_start(iw[0:16, :], flat.rearrange("(f p) -> p f", p=16))
        for g in range(1, 8):
            nc.sync.dma_start(iw[16 * g:16 * (g + 1), :], iw[0:16, :])
        return iw

    # knn1 matmuls
    def mm1(pe, t, c):
        msl = slice(128 * t, 128 * (t + 1))
        csl = slice(512 * c, 512 * (c + 1))
        nc.tensor.matmul(pe[:, csl], lhs_hi1[:, msl], rhs_hi1[:, csl],
                         start=True, stop=False)
        nc.tensor.matmul(pe[:, csl], lhsx[:, msl], rhsx[:, csl],
                         start=False, stop=True)

    # ---------------- block 1, interleaved with knn1 ----------------
    # y1a[(n,j)] = W1a @ (nb - cen) + W1b @ cen ; stats; keep for BN.
    # Each knn1 tile's top-k feeds its 4 stats chunks immediately, so the
    # gather-paced loop overlaps the DVE top-k of later tiles.
    y1a_d = [dram.tile([P, S], dt.float32, name="y1a_d0"),
             dram.tile([24, S], dt.float32, name="y1a_d1")]
    rows1a = (128, 24)
    sum1a = [ph1.tile([r, NCHUNK], dt.float32, name=f"s1a_{i}") for i, r in enumerate(rows1a)]
    sq1a = [ph1.tile([r, NCHUNK], dt.float32, name=f"q1a_{i}") for i, r in enumerate(rows1a)]
    # b1 pools pre-allocated so the gather stream overlaps knn1; iw1 built
    # per-tile so gather t only depends on top-k tile t
    iw1 = per.tile([P, S // 16], dt.uint16, name="iw_k1")
    b1p = tc.alloc_tile_pool(name="b1p", bufs=1, space="PSUM")
    b1s = tc.alloc_tile_pool(name="b1s", bufs=3)
    knn_pass(mm1, idx1_d, "k1", kp_bufs=1)

    b1q = {}
    def b1_gather(t):
        cols = slice(80 * t, 80 * (t + 1))
        flat_t = idx1_d[t].rearrange("p k -> (p k)")
        nc.sync.dma_start(iw1[0:16, cols], flat_t.rearrange("(f p) -> p f", p=16))
        nc.sync.dma_start(iw1[16:32, cols], iw1[0:16, cols])
        nc.sync.dma_start(iw1[32:64, cols], iw1[0:32, cols])
        nc.sync.dma_start(iw1[64:128, cols], iw1[0:64, cols])
        nbr = b1s.tile([P, CB], dt.float32, name="nbr")
        for g in range(2):
            nc.gpsimd.indirect_copy(out=nbr[:, 640 * g:640 * (g + 1)],
                                    data=xstack[:].bitcast(dt.float32),
                                    idxs=iw1[:, 80 * t + 40 * g:80 * t + 40 * (g + 1)],
                                    i_know_ap_gather_is_preferred=True)
        nbxt = b1s.tile([P, CB], dt.float32r, name="nbxt")
        nc.scalar.copy(nbxt[:], nbr[:])
        b1q[t] = nbxt

    b1_gather(0)
    for t in range(NCHUNK):
        if t + 1 < NCHUNK:
            b1_gather(t + 1)
        nbx = b1q.pop(t)[:]
        csl = slice(CB * t, CB * (t + 1))
        for m, rows in enumerate(rows1a):
            msl = slice(128 * m, 128 * m + rows)
            yp = b1p.tile([P, CB], dt.float32, name="yp", space="PSUM")
            for c0, c1 in CSL:
                nc.tensor.matmul(yp[:rows, c0:c1], w1aT_hi[:, msl],
                                 nbx[0:64, c0:c1], start=True, stop=False)
                nc.tensor.matmul(yp[:rows, c0:c1], w1aX[:, msl],
                                 nbx[:, c0:c1], start=False, stop=True)
            uview = u1[m][:, 128 * t:128 * (t + 1), None].to_broadcast(
                [rows, 128, KNN])
            ob = b1s.tile([rows, CB], dt.float32, name="ob1")
            nc.vector.scalar_tensor_tensor(
                out=ob[:].rearrange("p (n k) -> p n k", k=KNN),
                in0=yp[:rows, :].rearrange("p (n k) -> p n k", k=KNN), scalar=1.0,
                in1=uview, op0=ALU.mult, op1=ALU.add,
                accum_out=sum1a[m][:, t:t + 1])
            nc.sync.dma_start(y1a_d[m][:, csl], ob[:])
            tr = b1t.tile([rows, CB], dt.bfloat16, name="tr")
            nc.scalar.activation(tr[:], ob[:], AF.Square,
                                 accum_out=sq1a[m][:, t:t + 1])
    b1s.release()
    b1p.release()
    k2pool = tc.alloc_tile_pool(name="k2pool", bufs=1)

    s1 = [_acc_reduce(nc, per, sum1a[m][:], rows1a[m], f"s1r{m}") for m in range(2)]
    q1 = [_acc_reduce(nc, per, sq1a[m][:], rows1a[m], f"q1r{m}") for m in range(2)]

    # h1 load prefetch issued before the AllReduce so DMA overlaps it
    sumh1 = [ph1.tile([r, NCHUNK], dt.float32, name=f"sh1_{i}") for i, r in enumerate(rows1a)]
    rows1b = (128, 53)
    m1 = [k2pool.tile([r, N], dt.float32, name=f"m1_{i}") for i, r in enumerate(rows1b)]
    sq1b = [ph1.tile([r, NCHUNK], dt.float32, name=f"q1b_{i}") for i, r in enumerate(rows1b)]
    b2pp = tc.alloc_tile_pool(name="b2pp", bufs=2, space="PSUM")
    h1l = tc.alloc_tile_pool(name="h1l", bufs=2)

    h1q = {}
    def h1_load(c):
        csl = slice(CB * c, CB * (c + 1))
        pair = []
        for m, rows in enumerate(rows1a):
            ld = h1l.tile([rows, CB], dt.float32, name=f"h1ld{m}")
            nc.sync.dma_start(ld[:], y1a_d[m][:, csl])
            pair.append(ld)
        h1q[c] = pair

    h1_load(0)
    ar1 = _allreduce(nc, dram, per, [s1[0][:], s1[1][:]], [q1[0][:], q1[1][:]],
                     rows1a, 152, "ar1")
    sb1a = [_bn_coeffs(nc, per, ar1[m][0][:], ar1[m][1][:], g1a[m][:], b1a[m][:],
                       rows1a[m], CNT, f"bn1a{m}") for m in range(2)]
    for c in range(NCHUNK):
        if c + 1 < NCHUNK:
            h1_load(c + 1)
        csl = slice(CB * c, CB * (c + 1))
        h1c = h1q.pop(c)
        for m, rows in enumerate(rows1a):
            ld = h1c[m]
            nc.scalar.activation(ld[:], ld[:], AF.Prelu,
                                 bias=sb1a[m][1][:], scale=sb1a[m][0][:], alpha=0.2,
                                 accum_out=sumh1[m][:, c:c + 1])
        for m, rows in enumerate(rows1b):
            msl = slice(128 * m, 128 * m + rows)
            yp = b2pp.tile([P, CB], dt.float32, name="y1bp", space="PSUM")
            for c0, c1 in CSL:
                nc.tensor.matmul(yp[:rows, c0:c1], wkt1b[0][:, msl],
                                 h1c[0][:, c0:c1], start=True, stop=False)
                nc.tensor.matmul(yp[:rows, c0:c1], wkt1b[1][:, msl],
                                 h1c[1][:, c0:c1], start=False, stop=True)
            tr = b1t.tile([rows, CB], dt.bfloat16, name="tr")
            nc.scalar.activation(tr[:], yp[:rows, :], AF.Square,
                                 accum_out=sq1b[m][:, c:c + 1])
            nc.vector.tensor_reduce(out=m1[m][:, 128 * c:128 * (c + 1)],
                                    in_=yp[:rows, :].rearrange("p (n k) -> p n k", k=KNN),
                                    axis=AX.X, op=ALU.max)
    b2pp.release()
    h1l.release()

    # sum(y1b) = W1b' @ sum(h1)
    sh1 = [_acc_reduce(nc, per, sumh1[m][:], rows1a[m], f"sh1r{m}") for m in range(2)]
    ps1b = tc.alloc_tile_pool(name="ps1b", bufs=1, space="PSUM")
    s1b = []
    for m, rows in enumerate(rows1b):
        msl = slice(128 * m, 128 * m + rows)
        sp = ps1b.tile([rows, 1], dt.float32, name=f"s1bp{m}", space="PSUM")
        nc.tensor.matmul(sp[:], wkt1b[0][:, msl], sh1[0][:], start=True, stop=False)
        nc.tensor.matmul(sp[:], wkt1b[1][:, msl], sh1[1][:], start=False, stop=True)
        st = per.tile([rows, 1], dt.float32, name=f"s1b_{m}")
        nc.scalar.copy(st[:], sp[:])
        s1b.append(st)
    q1b = [_acc_reduce(nc, per, sq1b[m][:], rows1b[m], f"q1br{m}") for m in range(2)]
    ps1b.release()
    ar2 = _allreduce(nc, dram, per, [s1b[0][:], s1b[1][:]], [q1b[0][:], q1b[1][:]],
                     rows1b, 181, "ar2")
    sb1b = [_bn_coeffs(nc, per, ar2[m][0][:], ar2[m][1][:], g1b[m][:], b1b[m][:],
                       rows1b[m], CNT, f"bn1b{m}") for m in range(2)]

    # x1 = prelu(bn(m1)) -> hi/lo splits + sq2, all chunk-wise
    pk2op = tc.alloc_tile_pool(name="pk2op", bufs=1)   # knn2 operands (freed w/ k2pool)
    ones53 = per.tile([53, 1], dt.float32r, name="ones53")
    nc.vector.tensor_copy(ones53[:], ones_f[:53, :])
    ones128 = per.tile([128, 1], dt.float32r, name="ones128")
    nc.vector.tensor_copy(ones128[:], ones_f[:])
    x1hi = [per.tile([r, N], dt.float32r, name=f"x1s{m}_hi") for m, r in enumerate(rows1b)]
    x1lo = [pk2op.tile([r, N], dt.bfloat16, name=f"x1s{m}_lo") for m, r in enumerate(rows1b)]
    r2h_k1 = pk2op.tile([55, N], dt.float32r, name="r2h_k1")
    psq2 = tc.alloc_tile_pool(name="psq2", bufs=1, space="PSUM")
    sqt = tc.alloc_tile_pool(name="sqt", bufs=1)
    sq2p = psq2.tile([1, N], dt.float32, name="sq2p", space="PSUM")
    for c in range(4):
        sl = slice(512 * c, 512 * (c + 1))
        first = True
        for m, rows in enumerate(rows1b):
            x1c = sqt.tile([rows, 512], dt.float32, name="x1c")
            nc.scalar.activation(x1c[:], m1[m][:, sl], AF.Prelu, bias=sb1b[m][1][:],
                                 scale=sb1b[m][0][:], alpha=0.2)
            nc.vector.tensor_copy(x1hi[m][:, sl], x1c[:])
            nc.vector.tensor_tensor(out=x1lo[m][:, sl], in0=x1c[:],
                                    in1=x1hi[m][:, sl].bitcast(dt.float32),
                                    op=ALU.subtract)
            t = sqt.tile([rows, 512], dt.float32, name="x1sqc")
            nc.vector.tensor_tensor(out=t[:], in0=x1c[:], in1=x1c[:], op=ALU.mult)
            h = sqt.tile([rows, 512], dt.float32r, name="x1sqh")
            nc.vector.tensor_copy(h[:], t[:])
            l = sqt.tile([rows, 512], dt.float32r, name="x1sql")
            nc.vector.tensor_tensor(out=l[:], in0=t[:], in1=h[:].bitcast(dt.float32),
                                    op=ALU.subtract)
            on = ones128 if rows == 128 else ones53
            nc.tensor.matmul(sq2p[:, sl], on[:], h[:], start=first, stop=False)
            first = False
            nc.tensor.matmul(sq2p[:, sl], on[:], l[:],
                             start=False, stop=(m == 1))
        sc_ = sqt.tile([1, 512], dt.float32, name="sq2c")
        nc.scalar.copy(sc_[:], sq2p[:, sl])
        sh_ = sqt.tile([1, 512], dt.float32r, name="sq2ch")
        nc.vector.tensor_copy(sh_[:], sc_[:])
        sl_ = sqt.tile([1, 512], dt.float32r, name="sq2cl")
        nc.vector.tensor_tensor(out=sl_[:], in0=sc_[:], in1=sh_[:].bitcast(dt.float32),
                                op=ALU.subtract)
        nh_ = sqt.tile([1, 512], dt.float32r, name="sq2nh")
        nc.vector.tensor_scalar_mul(nh_[:], sh_[:], -1.0)
        nc.sync.dma_start(r2h_k1[53:54, sl], nh_[:])
        nl_ = sqt.tile([1, 512], dt.float32r, name="sq2nl")
        nc.vector.tensor_scalar_mul(nl_[:], sl_[:], -1.0)
        nc.sync.dma_start(r2h_k1[54:55, sl], nl_[:])
    psq2.release()
    sqt.release()

    # knn2 operands: lhsT reuses x1hi/x1lo; only scaled rhs tiles are built
    l2h_k1 = pk2op.tile([55, N], dt.float32r, name="l2h_k1")    # x1hi 128:181 + 2 ones
    nc.vector.tensor_copy(l2h_k1[0:53, :], x1hi[1][:])
    ones2f2 = pk2op.tile([2, N], dt.float32, name="ones2f2")
    nc.vector.memset(ones2f2[:], 1.0)
    ones2n = pk2op.tile([2, N], dt.float32r, name="ones2n")
    nc.vector.tensor_copy(ones2n[:], ones2f2[:])
    nc.sync.dma_start(l2h_k1[53:55, :], ones2n[:])
    r2h_k0 = pk2op.tile([P, N], dt.float32r, name="r2h_k0")     # 2*x1hi[0]
    nc.vector.tensor_scalar_mul(r2h_k0[:], x1hi[0][:], 2.0)
    nc.vector.tensor_scalar_mul(r2h_k1[0:53, :], x1hi[1][:], 2.0)
    r2l0 = pk2op.tile([P, N], dt.bfloat16, name="r2l0")         # 2*x1lo[0]
    nc.vector.tensor_scalar_mul(r2l0[:], x1lo[0][:], 2.0)
    r2l1 = pk2op.tile([53, N], dt.bfloat16, name="r2l1")        # 2*x1lo[1]
    nc.vector.tensor_scalar_mul(r2l1[:], x1lo[1][:], 2.0)
    # bf16 copies of hi operands for the lo-correction matmuls (1 cyc/col)
    hiB0 = pk2op.tile([P, N], dt.bfloat16, name="hiB0")
    nc.vector.tensor_copy(hiB0[:], x1hi[0][:].bitcast(dt.float32))
    hiB1 = pk2op.tile([53, N], dt.bfloat16, name="hiB1")
    nc.vector.tensor_copy(hiB1[:], x1hi[1][:].bitcast(dt.float32))
    r2hB0 = pk2op.tile([P, N], dt.bfloat16, name="r2hB0")
    nc.vector.tensor_copy(r2hB0[:], r2h_k0[:].bitcast(dt.float32))
    r2hB1 = pk2op.tile([53, N], dt.bfloat16, name="r2hB1")
    nc.vector.tensor_copy(r2hB1[:], r2h_k1[0:53, :].bitcast(dt.float32))

    def mm2(pe, t, c):
        msl = slice(128 * t, 128 * (t + 1))
        csl = slice(512 * c, 512 * (c + 1))
        nc.tensor.matmul(pe[:, csl], x1hi[0][:, msl], r2h_k0[:, csl], start=True, stop=False)
        nc.tensor.matmul(pe[:, csl], l2h_k1[:, msl], r2h_k1[:, csl], start=False, stop=False)
        nc.tensor.matmul(pe[:, csl], hiB0[:, msl], r2l0[:, csl], start=False, stop=False)
        nc.tensor.matmul(pe[:, csl], hiB1[:, msl], r2l1[:, csl], start=False, stop=False)
        nc.tensor.matmul(pe[:, csl], x1lo[0][:, msl], r2hB0[:, csl], start=False, stop=False)
        nc.tensor.matmul(pe[:, csl], x1lo[1][:, msl], r2hB1[:, csl], start=False, stop=True)

    knn_pass(mm2, idx2_d, "k2")
    iw2 = build_wrapped(idx2_d, "k2")
    pk2op.release()
    k2pool.release()
    ph1.release()

    # ---------------- block 2 ----------------
    # conv2b weights early so their DMA overlaps b2a / AR3
    b2bpool = tc.alloc_tile_pool(name="b2bpool", bufs=1)
    w2bT_r, w2bT_b = [], []
    for i, rows in enumerate((128, 128, 128, 46)):
        wt = wld.tile([128, 595], dt.float32, name="wf")
        nc.sync.dma_start(wt[:rows, :512], ins[f"w2bT_k{i}"])
        wr = b2bpool.tile([rows, 512], dt.float32r, name=f"w2bT_r{i}")
        nc.vector.tensor_copy(wr[:], wt[:rows, :512])
        w2bT_r.append(wr)
        wb = b2bpool.tile([rows, 512], dt.bfloat16, name=f"w2bT_b{i}")
        nc.vector.tensor_copy(wb[:], wt[:rows, :512])
        w2bT_b.append(wb)

    # U2[c, n] = sum_ci du2[ci, c] x1[ci, n]   (plain f32r)
    b2pool = tc.alloc_tile_pool(name="b2pool", bufs=1)
    w2aT_r, du2T_r = [], []
    for i, rows in enumerate((128, 53)):
        wt = wld.tile([128, 595], dt.float32, name="wf")
        nc.sync.dma_start(wt[:rows, :430], ins[f"w2aT_k{i}"])
        wr = b2pool.tile([rows, 430], dt.float32r, name=f"w2aT_r{i}")
        nc.vector.tensor_copy(wr[:], wt[:rows, :430])
        w2aT_r.append(wr)
        ddt = wld.tile([128, 595], dt.float32, name="wf")
        nc.sync.dma_start(ddt[:rows, :430], ins[f"du2T_k{i}"])
        dr = b2pool.tile([rows, 430], dt.float32r, name=f"du2T_r{i}")
        nc.vector.tensor_copy(dr[:], ddt[:rows, :430])
        du2T_r.append(dr)
    rows2a_ = (128, 128, 128, 46)
    u2 = [b2pool.tile([r, N], dt.float32, name=f"u2_{i}") for i, r in enumerate(rows2a_)]
    pu2 = tc.alloc_tile_pool(name="pu2", bufs=4, space="PSUM")
    for m, rows in enumerate(rows2a_):
        msl = slice(128 * m, 128 * m + rows)
        for cc in range(4):
            sl = slice(512 * cc, 512 * (cc + 1))
            up = pu2.tile([rows, 512], dt.float32, name="up2", space="PSUM")
            nc.tensor.matmul(up[:], du2T_r[0][:, msl], x1hi[0][:, sl], start=True, stop=False)
            nc.tensor.matmul(up[:], du2T_r[1][:, msl], x1hi[1][:, sl], start=False, stop=True)
            nc.scalar.copy(u2[m][:, sl], up[:])
    pu2.release()

    rows2a = (128, 128, 128, 46)
    y2a_d = [dram.tile([r, S], dt.bfloat16, name=f"y2a_d{i}") for i, r in enumerate(rows2a)]
    sum2a = [b2pool.tile([r, NCHUNK], dt.float32, name=f"s2a_{i}") for i, r in enumerate(rows2a)]
    sq2a = [b2pool.tile([r, NCHUNK], dt.float32, name=f"q2a_{i}") for i, r in enumerate(rows2a)]

    # gather sources: bf16 copies of x1 (bf16 conv2a avoids f32r rounding casts)
    gx0 = b2pool.tile([P, N], dt.bfloat16, name="gx0")
    nc.vector.tensor_copy(gx0[:], x1hi[0][:].bitcast(dt.float32))
    gx1 = b2pool.tile([P, N], dt.bfloat16, name="gx1")
    nc.vector.tensor_copy(gx1[:], gx0[:])
    nc.vector.tensor_copy(gx1[0:53, :], x1hi[1][:].bitcast(dt.float32))
    gxi = b2pool.tile([P, 2 * N], dt.bfloat16, name="gxi")
    _g3 = gxi[:].rearrange("p (n two) -> p n two", two=2)
    nc.vector.tensor_copy(_g3[:, :, 0], gx0[:])
    nc.vector.tensor_copy(_g3[:, :, 1], gx1[:])
    w2aT_b = []
    for i, rows in enumerate((128, 53)):
        wb = b2pool.tile([rows, 430], dt.bfloat16, name=f"w2aT_b{i}")
        nc.vector.tensor_copy(wb[:], w2aT_r[i][:].bitcast(dt.float32))
        w2aT_b.append(wb)

    b2p = tc.alloc_tile_pool(name="b2p", bufs=2, space="PSUM")
    b2s = tc.alloc_tile_pool(name="b2s", bufs=4)
    b2o = tc.alloc_tile_pool(name="b2o", bufs=4)
    gxi3 = gxi[:].rearrange("p (n two) -> p n two", two=2)
    iw2d = b2pool.tile([P, S // 16], dt.uint16, name="iw2d")
    nc.vector.tensor_scalar_mul(iw2d[:], iw2[:], 2.0)
    b2q = {}
    def b2_gather(c):
        nbi = b2s.tile([P, 2 * CB], dt.bfloat16, name="nbi")
        nbi3 = nbi[:].rearrange("p (n two) -> p n two", two=2)
        for g in range(4):
            nc.gpsimd.indirect_copy(
                out=nbi3[:, 320 * g:320 * (g + 1), :], data=gxi3,
                idxs=iw2d[:, 80 * c + 20 * g:80 * c + 20 * (g + 1)],
                i_know_ap_gather_is_preferred=True)
        nb0r = b2s.tile([P, CB], dt.bfloat16, name="nb0r")
        nc.vector.tensor_copy(nb0r[:], nbi3[:, :, 0])
        nb1r = b2s.tile([P, CB], dt.bfloat16, name="nb1r")
        nc.vector.tensor_copy(nb1r[:], nbi3[:, :, 1])
        b2q[c] = (nb0r, nb1r)

    b2_gather(0)
    b2_gather(1)
    for c in range(NCHUNK):
        if c + 2 < NCHUNK:
            b2_gather(c + 2)
        nb0r, nb1r = b2q.pop(c)
        for m, rows in enumerate(rows2a):
            msl = slice(128 * m, 128 * m + rows)
            yp = b2p.tile([P, CB], dt.float32, name="y2ap", space="PSUM")
            for c0, c1 in CSL:
                nc.tensor.matmul(yp[:rows, c0:c1], w2aT_b[0][:, msl],
                                 nb0r[:, c0:c1], start=True, stop=False)
                nc.tensor.matmul(yp[:rows, c0:c1], w2aT_b[1][:, msl],
                                 nb1r[0:53, c0:c1], start=False, stop=True)
            uview = u2[m][:, 128 * c:128 * (c + 1), None].to_broadcast(
                [rows, 128, KNN])
            ob = b2o.tile([rows, CB], dt.bfloat16, name="ob2")
            nc.vector.scalar_tensor_tensor(
                out=ob[:].rearrange("p (n k) -> p n k", k=KNN),
                in0=yp[:rows, :].rearrange("p (n k) -> p n k", k=KNN), scalar=1.0,
                in1=uview, op0=ALU.mult, op1=ALU.add,
                accum_out=sum2a[m][:, c:c + 1])
            nc.sync.dma_start(y2a_d[m][:, CB * c:CB * (c + 1)], ob[:])
            tr = b1t.tile([rows, CB], dt.bfloat16, name="tr")
            nc.scalar.activation(tr[:], ob[:], AF.Square,
                                 accum_out=sq2a[m][:, c:c + 1])
    b2p.release()
    b2o.release()
    b2s.release()

    s2 = [_acc_reduce(nc, per, sum2a[m][:], rows2a[m], f"s2r{m}") for m in range(4)]
    q2 = [_acc_reduce(nc, per, sq2a[m][:], rows2a[m], f"q2r{m}") for m in range(4)]
    b2pool.release()

    # block-3 weights early (LIFO: pw3 below b3p/b3l/c3t/px2r); DMA overlaps conv2b
    pw3 = tc.alloc_tile_pool(name="pw3", bufs=1)
    w3aT_r = []
    rows3a_k = (128, 53, 128, 128, 128, 128)
    for i, rows in enumerate(rows3a_k):
        wt = wld.tile([128, 595], dt.float32, name="wf")
        nc.sync.dma_start(wt[:rows, :595], ins[f"w3aT_k{i}"])
        wr = pw3.tile([rows, 595], dt.bfloat16, name=f"w3aT_r{i}")
        nc.vector.tensor_copy(wr[:], wt[:rows, :595])
        w3aT_r.append(wr)
    w3bT_r = []
    rows3b_k = (128, 128, 128, 128, 83)
    for i, rows in enumerate(rows3b_k):
        wt = wld.tile([128, 595], dt.float32, name="wf")
        nc.sync.dma_start(wt[:rows, :512], ins[f"w3bT_k{i}"])
        wr = pw3.tile([rows, 512], dt.bfloat16, name=f"w3bT_r{i}")
        nc.vector.tensor_copy(wr[:], wt[:rows, :512])
        w3bT_r.append(wr)
    pb3 = tc.alloc_tile_pool(name="pb3", bufs=1)   # y3a/y3b bf16 residency

    rows2b = (128, 128, 128, 128)
    sumh2 = [b2bpool.tile([r, NCHUNK], dt.float32, name=f"sh2_{i}") for i, r in enumerate(rows2a)]
    sq2b = [b2bpool.tile([r, NCHUNK], dt.float32, name=f"q2b_{i}") for i, r in enumerate(rows2b)]
    m2 = [b2bpool.tile([r, N], dt.bfloat16, name=f"m2_{i}") for i, r in enumerate(rows2b)]

    b3p = tc.alloc_tile_pool(name="b3p", bufs=2, space="PSUM")
    b3l = tc.alloc_tile_pool(name="b3l", bufs=3)

    c2q = {}
    def c2_load(c):
        csl = slice(CB * c, CB * (c + 1))
        grp = []
        for m, rows in enumerate(rows2a):
            ld = b3l.tile([rows, CB], dt.bfloat16, name=f"ld{m}")
            nc.sync.dma_start(ld[:], y2a_d[m][:, csl])
            grp.append(ld)
        c2q[c] = grp

    c2_load(0)
    ar3 = _allreduce(nc, dram, per, [s[:] for s in s2], [q[:] for q in q2],
                     rows2a, 430, "ar3")
    sb2a = [_bn_coeffs(nc, per, ar3[m][0][:], ar3[m][1][:], g2a[m][:], b2a[m][:],
                       rows2a[m], CNT, f"bn2a{m}") for m in range(4)]
    for c in range(NCHUNK):
        if c + 1 < NCHUNK:
            c2_load(c + 1)
        csl = slice(CB * c, CB * (c + 1))
        h2 = c2q.pop(c)
        for m, rows in enumerate(rows2a):
            ld = h2[m]
            nc.scalar.activation(ld[:], ld[:], AF.Prelu, bias=sb2a[m][1][:],
                                 scale=sb2a[m][0][:], alpha=0.2,
                                 accum_out=sumh2[m][:, c:c + 1])
        for m, rows in enumerate(rows2b):
            msl = slice(128 * m, 128 * (m + 1))
            yp = b3p.tile([P, CB], dt.float32, name="y2bp", space="PSUM")
            for c0, c1 in CSL:
                for k in range(4):
                    kr = (128, 128, 128, 46)[k]
                    nc.tensor.matmul(yp[:rows, c0:c1], w2bT_b[k][:, msl],
                                     h2[k][0:kr, c0:c1],
                                     start=(k == 0), stop=(k == 3))
            tr = b1t.tile([rows, CB], dt.bfloat16, name="tr")
            nc.scalar.activation(tr[:], yp[:rows, :], AF.Square,
                                 accum_out=sq2b[m][:, c:c + 1])
            nc.vector.tensor_reduce(out=m2[m][:, 128 * c:128 * (c + 1)],
                                    in_=yp[:rows, :].rearrange("p (n k) -> p n k", k=KNN),
                                    axis=AX.X, op=ALU.max)
    b3p.release()
    b3l.release()

    sh2 = [_acc_reduce(nc, per, sumh2[m][:], rows2a[m], f"sh2r{m}") for m in range(4)]
    sh2r = []
    for m in range(4):
        t = per.tile([rows2a[m], 4], dt.float32r, name=f"sh2rr{m}")
        nc.vector.tensor_copy(t[:], sh2[m][:].to_broadcast([rows2a[m], 4]))
        sh2r.append(t)
    ps2b = tc.alloc_tile_pool(name="ps2b", bufs=1, space="PSUM")
    s2b = []
    for m in range(4):
        msl = slice(128 * m, 128 * (m + 1))
        sp = ps2b.tile([128, 4], dt.float32, name=f"s2bp{m}", space="PSUM")
        for k in range(4):
            kr = (128, 128, 128, 46)[k]
            nc.tensor.matmul(sp[:], w2bT_r[k][:, msl], sh2r[k][0:kr, :],
                             start=(k == 0), stop=(k == 3))
        st = per.tile([128, 1], dt.float32, name=f"s2b_{m}")
        nc.scalar.copy(st[:], sp[:, 0:1])
        s2b.append(st)
    ps2b.release()
    q2b = [_acc_reduce(nc, per, sq2b[m][:], 128, f"q2br{m}") for m in range(4)]
    ar4 = _allreduce(nc, dram, per, [s[:] for s in s2b], [q[:] for q in q2b],
                     rows2b, 512, "ar4")
    sb2b = [_bn_coeffs(nc, per, ar4[m][0][:], ar4[m][1][:], g2b[m][:], b2b[m][:],
                       128, CNT, f"bn2b{m}") for m in range(4)]

    # x2 = prelu(bn(m2)) -> bf16; x1 recast to bf16 for block 3
    c3t = tc.alloc_tile_pool(name="c3t", bufs=4)
    px2r = tc.alloc_tile_pool(name="px2r", bufs=1)
    x2r = [px2r.tile([128, N], dt.bfloat16, name=f"x2r_{i}") for i in range(4)]
    for m in range(4):
        nc.scalar.activation(x2r[m][:], m2[m][:], AF.Prelu, bias=sb2b[m][1][:],
                             scale=sb2b[m][0][:], alpha=0.2)
    xb1 = [px2r.tile([r, N], dt.bfloat16, name=f"xb1_{i}")
           for i, r in enumerate((128, 53))]
    for i in range(2):
        nc.vector.tensor_copy(xb1[i][:], x1hi[i][:].bitcast(dt.float32))

    # ---------------- block 3 ----------------
    # y3a / y3b stay SBUF-resident in bf16 (post-x1 path tolerates bf16)
    xc_k = [xb1[0], xb1[1], x2r[0], x2r[1], x2r[2], x2r[3]]
    rows3a = (128, 128, 128, 128, 83)
    y3a_s = [pb3.tile([r, N], dt.bfloat16, name=f"y3a_s{i}") for i, r in enumerate(rows3a)]
    sum3a = [per.tile([r, 4], dt.float32, name=f"s3a_{i}") for i, r in enumerate(rows3a)]
    sq3a = [per.tile([r, 4], dt.float32, name=f"q3a_{i}") for i, r in enumerate(rows3a)]
    c3p = tc.alloc_tile_pool(name="c3p", bufs=4, space="PSUM")
    for c in range(4):
        csl = slice(512 * c, 512 * (c + 1))
        for m, rows in enumerate(rows3a):
            msl = slice(128 * m, 128 * m + rows)
            yp = c3p.tile([128, 512], dt.float32, name="y3ap", space="PSUM")
            for k in range(6):
                nc.tensor.matmul(yp[:rows, :], w3aT_r[k][:, msl], xc_k[k][:, csl],
                                 start=(k == 0), stop=(k == 5))
            nc.scalar.activation(y3a_s[m][:, csl], yp[:rows, :], AF.Copy,
                                 accum_out=sum3a[m][:, c:c + 1])
            tr = c3t.tile([rows, 512], dt.bfloat16, name="tr")
            nc.scalar.activation(tr[:], yp[:rows, :], AF.Square,
                                 accum_out=sq3a[m][:, c:c + 1])
    c3p.release()
    px2r.release()

    s3 = [_acc_reduce(nc, per, sum3a[m][:], rows3a[m], f"s3r{m}") for m in range(5)]
    q3 = [_acc_reduce(nc, per, sq3a[m][:], rows3a[m], f"q3r{m}") for m in range(5)]
    ar5 = _allreduce(nc, dram, per, [s[:] for s in s3], [q[:] for q in q3],
                     rows3a, 640, "ar5")
    sb3a = [_bn_coeffs(nc, per, ar5[m][0][:], ar5[m][1][:], g3a[m][:], b3a[m][:],
                       rows3a[m], CNT3, f"bn3a{m}") for m in range(5)]

    rows3b = (128, 128, 128, 128)
    y3b_s = [pb3.tile([r, N], dt.bfloat16, name=f"y3b_s{i}") for i, r in enumerate(rows3b)]
    sum3b = [per.tile([r, 4], dt.float32, name=f"s3b_{i}") for i, r in enumerate(rows3b)]
    sq3b = [per.tile([r, 4], dt.float32, name=f"q3b_{i}") for i, r in enumerate(rows3b)]
    c4p = tc.alloc_tile_pool(name="c4p", bufs=2, space="PSUM")
    h3l = tc.alloc_tile_pool(name="h3l", bufs=10)
    for c in range(4):
        csl = slice(512 * c, 512 * (c + 1))
        h3c = []
        for k, kr in enumerate(rows3a):
            ld = h3l.tile([kr, 512], dt.bfloat16, name="h3ld")
            nc.scalar.activation(ld[:], y3a_s[k][:, csl], AF.Prelu,
                                 bias=sb3a[k][1][:], scale=sb3a[k][0][:], alpha=0.2)
            h3c.append(ld)
        for m, rows in enumerate(rows3b):
            msl = slice(128 * m, 128 * (m + 1))
            yp = c4p.tile([rows, 512], dt.float32, name=f"y3bp{m}", space="PSUM")
            for k in range(5):
                nc.tensor.matmul(yp[:], w3bT_r[k][:, msl], h3c[k][:],
                                 start=(k == 0), stop=(k == 4))
            nc.scalar.activation(y3b_s[m][:, csl], yp[:], AF.Copy,
                                 accum_out=sum3b[m][:, c:c + 1])
            tr = c3t.tile([rows, 512], dt.bfloat16, name="tr")
            nc.scalar.activation(tr[:], yp[:], AF.Square,
                                 accum_out=sq3b[m][:, c:c + 1])
    c4p.release()
    h3l.release()
    c3t.release()

    s4 = [_acc_reduce(nc, per, sum3b[m][:], 128, f"s4r{m}") for m in range(4)]
    q4 = [_acc_reduce(nc, per, sq3b[m][:], 128, f"q4r{m}") for m in range(4)]
    ar6 = _allreduce(nc, dram, per, [s[:] for s in s4], [q[:] for q in q4],
                     rows3b, 512, "ar6")
    sb3b = [_bn_coeffs(nc, per, ar6[m][0][:], ar6[m][1][:], g3b[m][:], b3b[m][:],
                       128, CNT3, f"bn3b{m}") for m in range(4)]

    out_d = nc._ext_outputs["out"]
    for m in range(4):
        for c in range(4):
            csl = slice(512 * c, 512 * (c + 1))
            ot = oseg.tile([128, 512], dt.float32, name="ot")
            nc.scalar.activation(ot[:], y3b_s[m][:, csl], AF.Prelu,
                                 bias=sb3b[m][1][:], scale=sb3b[m][0][:], alpha=0.2)
            nc.sync.dma_start(out_d[128 * m:128 * (m + 1), csl], ot[:])
    pb3.release()
    pw3.release()
    b2bpool.release()
    wld.release()
    oseg.release()
    b1t.release()
    per.release()


# ------------------------------------------------------------------ host side
_CACHE = {}


def _build():
    _install_bassfix()
    nc = bass.Bass("TRN2", target_bir_lowering=False, debug=False,
                   num_devices=NCORES)
    in_specs = {
        "x": (64, N), "w1aT": (64, 152), "du1T": (64, 152),
        "w1bT_k0": (128, 181), "w1bT_k1": (24, 181),
        "w2aT_k0": (128, 430), "w2aT_k1": (53, 430),
        "du2T_k0": (128, 430), "du2T_k1": (53, 430),
        "w2bT_k0": (128, 512), "w2bT_k1": (128, 512),
        "w2bT_k2": (128, 512), "w2bT_k3": (46, 512),
        "w3aT_k0": (128, 595), "w3aT_k1": (53, 595), "w3aT_k2": (128, 595),
        "w3aT_k3": (128, 595), "w3aT_k4": (128, 595), "w3aT_k5": (128, 595),
        "w3bT_k0": (128, 512), "w3bT_k1": (128, 512), "w3bT_k2": (128, 512),
        "w3bT_k3": (128, 512), "w3bT_k4": (83, 512),
        "g1a": (152, 1), "be1a": (152, 1), "g1b": (181, 1), "be1b": (181, 1),
        "g2a": (430, 1), "be2a": (430, 1), "g2b": (512, 1), "be2b": (512, 1),
        "g3a": (595, 1), "be3a": (595, 1), "g3b": (512, 1), "be3b": (512, 1),
        "E": (32, CB),
    }
    nc._ext_inputs = {}
    for nm, shp in in_specs.items():
        nc._ext_inputs[nm] = nc.dram_tensor(nm, list(shp), dt.float32,
                                            kind="ExternalInput").ap()
    nc._ext_outputs = {
        "out": nc.dram_tensor("out", [512, N], dt.float32,
                              kind="ExternalOutput").ap()}
    with tile.TileContext(nc) as tc:
        build_kernel(nc, tc)
    return nc


def _host_inputs(x, c1w1, c1g1, c1be1, c1w2, c1g2, c1be2,
                 c2w1, c2g1, c2be1, c2w2, c2g2, c2be2,
                 c3w1, c3g1, c3be1, c3w2, c3g2, c3be2):
    f32 = np.float32
    W1 = np.asarray(c1w1, f32)            # [152, 128]
    W1a, W1b = W1[:, :64], W1[:, 64:]
    W2 = np.asarray(c2w1, f32)            # [430, 362]
    W2a, W2b = W2[:, :181], W2[:, 181:]
    ws = {
        "w1aT": np.ascontiguousarray(W1a.T),
        "du1T": np.ascontiguousarray((W1b - W1a).T),
    }
    w1bT = np.ascontiguousarray(np.asarray(c1w2, f32).T)     # [152, 181]
    ws["w1bT_k0"], ws["w1bT_k1"] = w1bT[:128], w1bT[128:]
    w2aT = np.ascontiguousarray(W2a.T)                        # [181, 430]
    du2T = np.ascontiguousarray((W2b - W2a).T)
    ws["w2aT_k0"], ws["w2aT_k1"] = w2aT[:128], w2aT[128:]
    ws["du2T_k0"], ws["du2T_k1"] = du2T[:128], du2T[128:]
    w2bT = np.ascontiguousarray(np.asarray(c2w2, f32).T)     # [430, 512]
    for i, sl in enumerate((slice(0, 128), slice(128, 256), slice(256, 384),
                            slice(384, 430))):
        ws[f"w2bT_k{i}"] = w2bT[sl]
    w3aT = np.ascontiguousarray(np.asarray(c3w1, f32).T)     # [693, 595]
    cuts = (0, 128, 181, 309, 437, 565, 693)
    for i in range(6):
        ws[f"w3aT_k{i}"] = w3aT[cuts[i]:cuts[i + 1]]
    w3bT = np.ascontiguousarray(np.asarray(c3w2, f32).T)     # [595, 512]
    for i in range(5):
        ws[f"w3bT_k{i}"] = w3bT[128 * i:min(128 * (i + 1), 595)]
    for nm, v in (("g1a", c1g1), ("be1a", c1be1), ("g1b", c1g2), ("be1b", c1be2),
                  ("g2a", c2g1), ("be2a", c2be1), ("g2b", c2g2), ("be2b", c2be2),
                  ("g3a", c3g1), ("be3a", c3be1), ("g3b", c3g2), ("be3b", c3be2)):
        ws[nm] = np.ascontiguousarray(np.asarray(v, f32).reshape(-1, 1))
    E = np.zeros((32, CB), f32)
    for r in range(32):
        E[r, KNN * r:KNN * (r + 1)] = 1.0
    ws["E"] = E
    ws = {k: np.ascontiguousarray(v, f32) for k, v in ws.items()}
    xs = np.asarray(x, f32)
    in_maps = []
    for i in range(NCORES):
        m = dict(ws)
        m["x"] = np.ascontiguousarray(xs[i])
        in_maps.append(m)
    return in_maps


def kernel(x, c1w1, c1b1, c1g1, c1be1, c1w2, c1b2, c1g2, c1be2,
           c2w1, c2b1, c2g1, c2be1, c2w2, c2b2, c2g2, c2be2,
           c3w1, c3b1, c3g1, c3be1, c3w2, c3b2, c3g2, c3be2):
    # conv biases are absorbed exactly by the following BatchNorm (shift
    # invariance), so b* inputs are unused by the device program.
    if "nc" not in _CACHE:
        _CACHE["nc"] = _build()
    nc = _CACHE["nc"]
    in_maps = _host_inputs(x, c1w1, c1g1, c1be1, c1w2, c1g2, c1be2,
                           c2w1, c2g1, c2be1, c2w2, c2g2, c2be2,
                           c3w1, c3g1, c3be1, c3w2, c3g2, c3be2)
    res = run_bass_kernel_spmd(nc, in_maps, core_ids=list(range(NCORES)))
    _CACHE["last_result"] = res
    out = np.stack([res.results[i]["out"] for i in range(NCORES)], axis=0)
    return out.astype(np.float32)



# revision 34
# speedup vs baseline: 1.1056x; 1.0152x over previous
"""DGCNN-style GCN encoder on 8 TRN2 NeuronCores (bass/tile).

Data-parallel over batch: each core owns one sample (B=8). BatchNorm batch
statistics are all-reduced across the 8 cores so results match global-batch
BN exactly. kNN top-10, neighbor gathers, and all convs/BN/activations run
on-device; the host only reshapes/replicates weights.

Precision strategy: matmuls on the x -> x1 -> knn2 path use a compensated
fp32r (hi/lo split) scheme that recovers ~fp32 accuracy at fp32r speed
(conv1b uses plain fp32). This keeps the kNN neighbor selection stable
against the f32 reference. Everything downstream of x1 (conv2a/2b, block 3)
runs in bf16 (weights + activations), which the 2e-2 tolerance absorbs.

Performance notes (vs the first working version):
- edge loops process 1280-edge chunks (128 points x 10 nbrs) instead of 320
- neighbor gathers are indirect_copy, whose cost is ~28ns per index of Q7
  dispatch; conv2a gathers both x1 row-tiles in ONE pass by interleaving
  them as bf16 pairs (d=2 gather with doubled indices), halving index count
- gathers/loads are software-pipelined one-to-two chunks ahead, and the
  next phase's first loads + weight loads are issued before each BN
  AllReduce so DMA overlaps the collective
- y3a/y3b stay SBUF-resident in bf16 (no DRAM round-trip); y2a is spilled
  to DRAM in bf16 (half the traffic)
"""

import os
import sys
import json

B16 = "/nix/store/wxap7svlj45h0lfm31d1axjjnzyl6qsy-b16-bazel-unstable-cc-2026-05-04-9a3fa1f3-rt-2026-05-04-ade39e0a/lib/python3.13/site-packages"
if B16 not in sys.path:
    sys.path.insert(0, B16)
if "/opt/trn_rl_repo" not in sys.path:
    sys.path.insert(0, "/opt/trn_rl_repo")

import numpy as np
import concourse.bass as bass
import concourse.mybir as mybir
import concourse.tile as tile
from concourse.bass_utils import run_bass_kernel_spmd

dt = mybir.dt
AF = mybir.ActivationFunctionType
ALU = mybir.AluOpType
AX = mybir.AxisListType

N = 2048
KNN = 10
S = N * KNN            # 20480 edge positions
CB = 1280              # banded chunk: 128 points x 10 neighbors
NCHUNK = S // CB       # 16
CSL = [(0, 512), (512, 1024), (1024, 1280)]   # matmul col slices of a chunk
EPS = 1e-5
NCORES = 8
CNT = float(NCORES * S)      # BN count for edge tensors
CNT3 = float(NCORES * N)     # BN count for block-3 tensors


# ---------------------------------------------------------------- walrus fix
def _fix_bir_json(bir_json: bytes) -> bytes:
    """This container's walrus accepts only ONE sync-wait per instruction;
    hoist extra waits onto preceding EventSemaphore instructions."""
    m = json.loads(bir_json)
    for f in m["functions"]:
        for bb in f["blocks"]:
            out = []
            for i in bb["instructions"]:
                si = i.get("sync_info") or {}
                waits = si.get("on_wait") or []
                if len(waits) > 1:
                    for k, w in enumerate(waits[:-1]):
                        out.append({
                            "name": f"{i['name']}w{k}",
                            "engine": i["engine"],
                            "opcode": "EventSemaphore",
                            "ins": [], "outs": [],
                            "debug": i.get("debug"),
                            "sync_info": {"on_update": [], "on_wait": [w]},
                        })
                    i = dict(i)
                    si = dict(si)
                    si["on_wait"] = [waits[-1]]
                    i["sync_info"] = si
                out.append(i)
            bb["instructions"] = out
    return json.dumps(m).encode()


def _install_bassfix():
    import concourse.bass_utils as bu
    import concourse.bass2jax as b2j
    if getattr(bu, "_bassfix_installed", False):
        return
    real = bu.compile_bir_kernel

    def patched(bir_json, tmpdir, neff_name="file.neff"):
        return real(_fix_bir_json(bir_json), tmpdir, neff_name)

    bu.compile_bir_kernel = patched
    b2j.compile_bir_kernel = patched
    bu._bassfix_installed = True


# ------------------------------------------------------------- device helpers
def _round_split(nc, pool, src_ap, parts, width, name):
    """Return (hi, lo) float32r tiles for a f32 source AP [parts, width]."""
    hi = pool.tile([parts, width], dt.float32r, name=name + "_hi")
    lo = pool.tile([parts, width], dt.float32r, name=name + "_lo")
    nc.vector.tensor_copy(hi[:], src_ap)
    nc.vector.tensor_tensor(out=lo[:], in0=src_ap, in1=hi[:].bitcast(dt.float32),
                            op=ALU.subtract)
    return hi, lo


def _bn_coeffs(nc, pool, sums, sqs, g_ap, b_ap, rows, cnt, name):
    """sums/sqs: [rows,1] f32 (global). Returns (scale, bias) [rows,1] f32."""
    mean = pool.tile([rows, 1], dt.float32, name=name + "_mean")
    nc.vector.tensor_scalar_mul(mean[:], sums, 1.0 / cnt)
    e2 = pool.tile([rows, 1], dt.float32, name=name + "_e2")
    nc.vector.tensor_scalar_mul(e2[:], sqs, 1.0 / cnt)
    m2 = pool.tile([rows, 1], dt.float32, name=name + "_m2")
    nc.vector.tensor_tensor(out=m2[:], in0=mean[:], in1=mean[:], op=ALU.mult)
    var = pool.tile([rows, 1], dt.float32, name=name + "_var")
    nc.vector.tensor_tensor(out=var[:], in0=e2[:], in1=m2[:], op=ALU.subtract)
    epst = pool.tile([rows, 1], dt.float32, name=name + "_eps")
    nc.vector.memset(epst[:], float(EPS))
    std = pool.tile([rows, 1], dt.float32, name=name + "_std")
    nc.scalar.activation(std[:], var[:], AF.Sqrt, bias=epst[:])
    rstd = pool.tile([rows, 1], dt.float32, name=name + "_rstd")
    nc.vector.reciprocal(rstd[:], std[:])
    scale = pool.tile([rows, 1], dt.float32, name=name + "_scale")
    nc.vector.tensor_tensor(out=scale[:], in0=rstd[:], in1=g_ap, op=ALU.mult)
    ms = pool.tile([rows, 1], dt.float32, name=name + "_ms")
    nc.vector.tensor_tensor(out=ms[:], in0=mean[:], in1=scale[:], op=ALU.mult)
    bias = pool.tile([rows, 1], dt.float32, name=name + "_bias")
    nc.vector.tensor_tensor(out=bias[:], in0=b_ap, in1=ms[:], op=ALU.subtract)
    return scale, bias


def _allreduce(nc, dram, per, sums_list, sqs_list, rows_list, cpad, name):
    """Pack per-Mtile [rows,1] sum/sq into a DRAM [cpad,2] buffer, AllReduce
    across the 8 cores, unpack back into fresh [rows,1] tiles."""
    cin = dram.tile([cpad, 2], dt.float32, name=name + "_in")
    cout = dram.tile([cpad, 2], dt.float32, name=name + "_out")
    r0 = 0
    for s_, q_, rows in zip(sums_list, sqs_list, rows_list):
        nc.sync.dma_start(cin[r0:r0 + rows, 0:1], s_)
        nc.sync.dma_start(cin[r0:r0 + rows, 1:2], q_)
        r0 += rows
    if r0 < cpad:
        z = per.tile([cpad - r0, 2], dt.float32, name=name + "_z")
        nc.vector.memset(z[:], 0.0)
        nc.sync.dma_start(cin[r0:cpad, :], z[:])
    nc.gpsimd.collective_compute(
        "AllReduce", ALU.add, replica_groups=[list(range(NCORES))],
        ins=[cin[:].opt()], outs=[cout[:].opt()])
    outs = []
    r0 = 0
    for rows in rows_list:
        gs = per.tile([rows, 1], dt.float32, name=name + f"_gs{r0}")
        gq = per.tile([rows, 1], dt.float32, name=name + f"_gq{r0}")
        nc.sync.dma_start(gs[:], cout[r0:r0 + rows, 0:1])
        nc.sync.dma_start(gq[:], cout[r0:r0 + rows, 1:2])
        outs.append((gs, gq))
        r0 += rows
    return outs


def _acc_reduce(nc, per, acc, rows, name):
    """[rows, NCHUNK] accumulator -> [rows,1]."""
    out = per.tile([rows, 1], dt.float32, name=name)
    nc.vector.tensor_reduce(out=out[:], in_=acc, axis=AX.X, op=ALU.add)
    return out


def build_kernel(nc, tc):
    P = 128
    per = tc.alloc_tile_pool(name="persist", bufs=1)
    dram = tc.alloc_tile_pool(name="dram", bufs=1, space="DRAM")
    b1t = tc.alloc_tile_pool(name="b1t", bufs=2)
    oseg = tc.alloc_tile_pool(name="oseg", bufs=1)
    wld = tc.alloc_tile_pool(name="wld", bufs=2)
    ph1 = tc.alloc_tile_pool(name="ph1", bufs=1)

    ins = nc._ext_inputs
    x_d = ins["x"]

    # ---------------- load & prep ----------------
    tmp0 = tc.alloc_tile_pool(name="tmp0", bufs=1)      # until U1t/rhs built
    x = tmp0.tile([64, N], dt.float32, name="x")
    nc.sync.dma_start(x[:], x_d)
    xhi, xlo = _round_split(nc, tmp0, x[:], 64, N, "x")
    # stacks for compensated matmuls
    xstack = ph1.tile([P, N], dt.float32r, name="xstack")   # [xhi; xlo]
    nc.vector.tensor_copy(xstack[:64, :], xhi[:])
    nc.vector.tensor_copy(xstack[64:, :], xlo[:])
    lhsx = xstack                                             # [xhi; xlo]
    rhsx = ph1.tile([P, N], dt.float32r, name="rhsx")        # 2*[xlo; xhi]
    nc.vector.tensor_scalar_mul(rhsx[:64, :], xlo[:], 2.0)
    nc.vector.tensor_scalar_mul(rhsx[64:, :], xhi[:], 2.0)

    # sq1 = sum_c x^2 (compensated)
    xsq = tmp0.tile([64, N], dt.float32, name="xsq")
    nc.vector.tensor_tensor(out=xsq[:], in0=x[:], in1=x[:], op=ALU.mult)
    xsqhi, xsqlo = _round_split(nc, tmp0, xsq[:], 64, N, "xsq")
    ones_f = per.tile([128, 1], dt.float32, name="ones_f")
    nc.vector.memset(ones_f[:], 1.0)
    ones64 = per.tile([64, 1], dt.float32r, name="ones64")
    nc.vector.tensor_copy(ones64[:], ones_f[:64, :])
    # knn1 matmul operands (sq1 folded in chunk-wise)
    lhs_hi1 = ph1.tile([66, N], dt.float32r, name="lhs_hi1")
    nc.vector.tensor_copy(lhs_hi1[:64, :], xhi[:])
    ones2f = tmp0.tile([2, N], dt.float32, name="ones2f")
    nc.vector.memset(ones2f[:], 1.0)
    nc.vector.tensor_copy(lhs_hi1[64:66, :], ones2f[:])
    rhs_hi1 = ph1.tile([66, N], dt.float32r, name="rhs_hi1")
    nc.vector.tensor_scalar_mul(rhs_hi1[:64, :], xhi[:], 2.0)
    psq = tc.alloc_tile_pool(name="psq", bufs=1, space="PSUM")
    sq1p = psq.tile([1, N], dt.float32, name="sq1p", space="PSUM")
    for c in range(4):
        sl = slice(512 * c, 512 * (c + 1))
        nc.tensor.matmul(sq1p[:, sl], ones64[:], xsqhi[:, sl], start=True, stop=False)
        nc.tensor.matmul(sq1p[:, sl], ones64[:], xsqlo[:, sl], start=False, stop=True)
        sc_ = tmp0.tile([1, 512], dt.float32, name="sq1c")
        nc.scalar.copy(sc_[:], sq1p[:, sl])
        sh_ = tmp0.tile([1, 512], dt.float32r, name="sq1ch")
        nc.vector.tensor_copy(sh_[:], sc_[:])
        sl_ = tmp0.tile([1, 512], dt.float32r, name="sq1cl")
        nc.vector.tensor_tensor(out=sl_[:], in0=sc_[:], in1=sh_[:].bitcast(dt.float32),
                                op=ALU.subtract)
        nc.vector.tensor_scalar_mul(rhs_hi1[64:65, sl], sh_[:], -1.0)
        nl_ = tmp0.tile([1, 512], dt.float32r, name="sq1nl")
        nc.vector.tensor_scalar_mul(nl_[:], sl_[:], -1.0)
        nc.sync.dma_start(rhs_hi1[65:66, sl], nl_[:])
    psq.release()

    # conv1 weights
    w1aT = wld.tile([128, 595], dt.float32, name="wf")
    nc.sync.dma_start(w1aT[:64, :152], ins["w1aT"])
    w1aT_hi, w1aT_lo = _round_split(nc, per, w1aT[:64, :152], 64, 152, "w1aT")
    du1T = wld.tile([128, 595], dt.float32, name="wf")
    nc.sync.dma_start(du1T[:64, :152], ins["du1T"])
    du1T_hi, du1T_lo = _round_split(nc, per, du1T[:64, :152], 64, 152, "du1T")
    w1aX = per.tile([P, 152], dt.float32r, name="w1aX")      # [Wlo; Whi]
    nc.vector.tensor_copy(w1aX[:64, :], w1aT_lo[:])
    nc.vector.tensor_copy(w1aX[64:, :], w1aT_hi[:])
    du1X = per.tile([P, 152], dt.float32r, name="du1X")      # [dlo; dhi] pairs lhsx
    nc.vector.tensor_copy(du1X[:64, :], du1T_lo[:])
    nc.vector.tensor_copy(du1X[64:, :], du1T_hi[:])

    wkt1b = []  # conv1b fp32 weights, K-tiles
    for i, rows in enumerate((128, 24)):
        t = per.tile([rows, 181], dt.float32, name=f"w1bT_k{i}")
        nc.sync.dma_start(t[:], ins[f"w1bT_k{i}"])
        wkt1b.append(t)


    # U1[c, n] = sum_ci du1[ci, c] x[ci, n]   (compensated, natural layout)
    rows1a_ = (128, 24)
    u1 = [ph1.tile([r, N], dt.float32, name=f"u1_{i}") for i, r in enumerate(rows1a_)]
    pu = tc.alloc_tile_pool(name="pu", bufs=4, space="PSUM")
    for m, rows in enumerate(rows1a_):
        msl = slice(128 * m, 128 * m + rows)
        for cc in range(4):
            sl = slice(512 * cc, 512 * (cc + 1))
            up = pu.tile([rows, 512], dt.float32, name="up", space="PSUM")
            nc.tensor.matmul(up[:], du1T_hi[:, msl], xhi[:, sl], start=True, stop=False)
            nc.tensor.matmul(up[:], du1X[:, msl], xstack[:, sl], start=False, stop=True)
            nc.scalar.copy(u1[m][:, sl], up[:])
    pu.release()
    tmp0.release()

    # BN params
    def load_vec(nm, rows_list):
        tiles = []
        r0 = 0
        for i, rows in enumerate(rows_list):
            t = per.tile([rows, 1], dt.float32, name=f"{nm}_{i}")
            nc.sync.dma_start(t[:], ins[nm][r0:r0 + rows, :])
            tiles.append(t)
            r0 += rows
        return tiles

    g1a = load_vec("g1a", (128, 24)); b1a = load_vec("be1a", (128, 24))
    g1b = load_vec("g1b", (128, 53)); b1b = load_vec("be1b", (128, 53))
    g2a = load_vec("g2a", (128, 128, 128, 46)); b2a = load_vec("be2a", (128, 128, 128, 46))
    g2b = load_vec("g2b", (128,) * 4); b2b = load_vec("be2b", (128,) * 4)
    g3a = load_vec("g3a", (128,) * 4 + (83,)); b3a = load_vec("be3a", (128,) * 4 + (83,))
    g3b = load_vec("g3b", (128,) * 4); b3b = load_vec("be3b", (128,) * 4)

    # ---------------- kNN (shared machinery) ----------------
    idx1_d = dram.tile([16, P, KNN], dt.uint16, name="idx1_d")
    idx2_d = dram.tile([16, P, KNN], dt.uint16, name="idx2_d")

    def knn_pass(mm_emit, idx_dram, tag, after_tile=None, kp_bufs=2):
        """mm_emit(psum_slice, c) emits matmuls for columns 512c..512c+512."""
        kp = tc.alloc_tile_pool(name=f"knnp_{tag}", bufs=kp_bufs, space="PSUM")
        ks = tc.alloc_tile_pool(name=f"knns_{tag}", bufs=2)
        for t in range(16):
            pe = kp.tile([P, N], dt.float32, name="pe", space="PSUM")
            for c in range(4):
                mm_emit(pe, t, c)
            q = ks.tile([P, N], dt.float32, name="q")
            nc.scalar.copy(q[:], pe[:])
            v8 = ks.tile([P, 8], dt.float32, name="v8")
            i8 = ks.tile([P, 8], dt.uint16, name="i8")
            nc.vector.max(out=v8[:], in_=q[:])
            nc.vector.max_index(out=i8[:], in_max=v8[:], in_values=q[:])
            nc.vector.match_replace(out=q[:], in_to_replace=v8[:], in_values=q[:],
                                    imm_value=-1e30)
            v8b = ks.tile([P, 8], dt.float32, name="v8b")
            i8b = ks.tile([P, 8], dt.uint16, name="i8b")
            nc.vector.max(out=v8b[:], in_=q[:])
            nc.vector.max_index(out=i8b[:], in_max=v8b[:], in_values=q[:])
            idx = ks.tile([P, KNN], dt.uint16, name="idx")
            nc.vector.tensor_copy(idx[:, 0:7], i8[:, 1:8])
            nc.vector.tensor_copy(idx[:, 7:10], i8b[:, 0:3])
            nc.sync.dma_start(idx_dram[t], idx[:])
            if after_tile is not None:
                after_tile(t)
        kp.release()
        ks.release()

    def build_wrapped(idx_dram, tag):
        """DRAM flat idx (point-major) -> wrapped [128, S/16] uint16."""
        iw = per.tile([P, S // 16], dt.uint16, name=f"iw_{tag}")
        flat = idx_dram[:].rearrange("a p k -> (a p k)")
        nc.sync.dma_start(iw[0:16, :], flat.rearrange("(f p) -> p f", p=16))
        for g in range(1, 8):
            nc.sync.dma_start(iw[16 * g:16 * (g + 1), :], iw[0:16, :])
        return iw

    # knn1 matmuls
    def mm1(pe, t, c):
        msl = slice(128 * t, 128 * (t + 1))
        csl = slice(512 * c, 512 * (c + 1))
        nc.tensor.matmul(pe[:, csl], lhs_hi1[:, msl], rhs_hi1[:, csl],
                         start=True, stop=False)
        nc.tensor.matmul(pe[:, csl], lhsx[:, msl], rhsx[:, csl],
                         start=False, stop=True)

    # ---------------- block 1, interleaved with knn1 ----------------
    # y1a[(n,j)] = W1a @ (nb - cen) + W1b @ cen ; stats; keep for BN.
    # Each knn1 tile's top-k feeds its 4 stats chunks immediately, so the
    # gather-paced loop overlaps the DVE top-k of later tiles.
    y1a_d = [dram.tile([P, S], dt.float32, name="y1a_d0"),
             dram.tile([24, S], dt.float32, name="y1a_d1")]
    rows1a = (128, 24)
    sum1a = [ph1.tile([r, NCHUNK], dt.float32, name=f"s1a_{i}") for i, r in enumerate(rows1a)]
    sq1a = [ph1.tile([r, NCHUNK], dt.float32, name=f"q1a_{i}") for i, r in enumerate(rows1a)]
    # b1 pools pre-allocated so the gather stream overlaps knn1; iw1 built
    # per-tile so gather t only depends on top-k tile t
    iw1 = per.tile([P, S // 16], dt.uint16, name="iw_k1")
    b1p = tc.alloc_tile_pool(name="b1p", bufs=1, space="PSUM")
    b1s = tc.alloc_tile_pool(name="b1s", bufs=3)
    knn_pass(mm1, idx1_d, "k1", kp_bufs=1)

    b1q = {}
    def b1_gather(t):
        cols = slice(80 * t, 80 * (t + 1))
        flat_t = idx1_d[t].rearrange("p k -> (p k)")
        nc.sync.dma_start(iw1[0:16, cols], flat_t.rearrange("(f p) -> p f", p=16))
        nc.sync.dma_start(iw1[16:32, cols], iw1[0:16, cols])
        nc.sync.dma_start(iw1[32:64, cols], iw1[0:32, cols])
        nc.sync.dma_start(iw1[64:128, cols], iw1[0:64, cols])
        nbr = b1s.tile([P, CB], dt.float32, name="nbr")
        for g in range(2):
            nc.gpsimd.indirect_copy(out=nbr[:, 640 * g:640 * (g + 1)],
                                    data=xstack[:].bitcast(dt.float32),
                                    idxs=iw1[:, 80 * t + 40 * g:80 * t + 40 * (g + 1)],
                                    i_know_ap_gather_is_preferred=True)
        nbxt = b1s.tile([P, CB], dt.float32r, name="nbxt")
        nc.scalar.copy(nbxt[:], nbr[:])
        b1q[t] = nbxt

    b1_gather(0)
    for t in range(NCHUNK):
        if t + 1 < NCHUNK:
            b1_gather(t + 1)
        nbx = b1q.pop(t)[:]
        csl = slice(CB * t, CB * (t + 1))
        for m, rows in enumerate(rows1a):
            msl = slice(128 * m, 128 * m + rows)
            yp = b1p.tile([P, CB], dt.float32, name="yp", space="PSUM")
            for c0, c1 in CSL:
                nc.tensor.matmul(yp[:rows, c0:c1], w1aT_hi[:, msl],
                                 nbx[0:64, c0:c1], start=True, stop=False)
                nc.tensor.matmul(yp[:rows, c0:c1], w1aX[:, msl],
                                 nbx[:, c0:c1], start=False, stop=True)
            uview = u1[m][:, 128 * t:128 * (t + 1), None].to_broadcast(
                [rows, 128, KNN])
            ob = b1s.tile([rows, CB], dt.float32, name="ob1")
            nc.vector.scalar_tensor_tensor(
                out=ob[:].rearrange("p (n k) -> p n k", k=KNN),
                in0=yp[:rows, :].rearrange("p (n k) -> p n k", k=KNN), scalar=1.0,
                in1=uview, op0=ALU.mult, op1=ALU.add,
                accum_out=sum1a[m][:, t:t + 1])
            nc.sync.dma_start(y1a_d[m][:, csl], ob[:])
            tr = b1t.tile([rows, CB], dt.bfloat16, name="tr")
            nc.scalar.activation(tr[:], ob[:], AF.Square,
                                 accum_out=sq1a[m][:, t:t + 1])
    b1s.release()
    b1p.release()
    k2pool = tc.alloc_tile_pool(name="k2pool", bufs=1)

    s1 = [_acc_reduce(nc, per, sum1a[m][:], rows1a[m], f"s1r{m}") for m in range(2)]
    q1 = [_acc_reduce(nc, per, sq1a[m][:], rows1a[m], f"q1r{m}") for m in range(2)]

    # h1 load prefetch issued before the AllReduce so DMA overlaps it
    sumh1 = [ph1.tile([r, NCHUNK], dt.float32, name=f"sh1_{i}") for i, r in enumerate(rows1a)]
    rows1b = (128, 53)
    m1 = [k2pool.tile([r, N], dt.float32, name=f"m1_{i}") for i, r in enumerate(rows1b)]
    sq1b = [ph1.tile([r, NCHUNK], dt.float32, name=f"q1b_{i}") for i, r in enumerate(rows1b)]
    b2pp = tc.alloc_tile_pool(name="b2pp", bufs=2, space="PSUM")
    h1l = tc.alloc_tile_pool(name="h1l", bufs=2)

    h1q = {}
    def h1_load(c):
        csl = slice(CB * c, CB * (c + 1))
        pair = []
        for m, rows in enumerate(rows1a):
            ld = h1l.tile([rows, CB], dt.float32, name=f"h1ld{m}")
            nc.sync.dma_start(ld[:], y1a_d[m][:, csl])
            pair.append(ld)
        h1q[c] = pair

    h1_load(0)
    ar1 = _allreduce(nc, dram, per, [s1[0][:], s1[1][:]], [q1[0][:], q1[1][:]],
                     rows1a, 152, "ar1")
    sb1a = [_bn_coeffs(nc, per, ar1[m][0][:], ar1[m][1][:], g1a[m][:], b1a[m][:],
                       rows1a[m], CNT, f"bn1a{m}") for m in range(2)]
    for c in range(NCHUNK):
        if c + 1 < NCHUNK:
            h1_load(c + 1)
        csl = slice(CB * c, CB * (c + 1))
        h1c = h1q.pop(c)
        for m, rows in enumerate(rows1a):
            ld = h1c[m]
            nc.scalar.activation(ld[:], ld[:], AF.Prelu,
                                 bias=sb1a[m][1][:], scale=sb1a[m][0][:], alpha=0.2,
                                 accum_out=sumh1[m][:, c:c + 1])
        for m, rows in enumerate(rows1b):
            msl = slice(128 * m, 128 * m + rows)
            yp = b2pp.tile([P, CB], dt.float32, name="y1bp", space="PSUM")
            for c0, c1 in CSL:
                nc.tensor.matmul(yp[:rows, c0:c1], wkt1b[0][:, msl],
                                 h1c[0][:, c0:c1], start=True, stop=False)
                nc.tensor.matmul(yp[:rows, c0:c1], wkt1b[1][:, msl],
                                 h1c[1][:, c0:c1], start=False, stop=True)
            tr = b1t.tile([rows, CB], dt.bfloat16, name="tr")
            nc.scalar.activation(tr[:], yp[:rows, :], AF.Square,
                                 accum_out=sq1b[m][:, c:c + 1])
            nc.vector.tensor_reduce(out=m1[m][:, 128 * c:128 * (c + 1)],
                                    in_=yp[:rows, :].rearrange("p (n k) -> p n k", k=KNN),
                                    axis=AX.X, op=ALU.max)
    b2pp.release()
    h1l.release()

    # sum(y1b) = W1b' @ sum(h1)
    sh1 = [_acc_reduce(nc, per, sumh1[m][:], rows1a[m], f"sh1r{m}") for m in range(2)]
    ps1b = tc.alloc_tile_pool(name="ps1b", bufs=1, space="PSUM")
    s1b = []
    for m, rows in enumerate(rows1b):
        msl = slice(128 * m, 128 * m + rows)
        sp = ps1b.tile([rows, 1], dt.float32, name=f"s1bp{m}", space="PSUM")
        nc.tensor.matmul(sp[:], wkt1b[0][:, msl], sh1[0][:], start=True, stop=False)
        nc.tensor.matmul(sp[:], wkt1b[1][:, msl], sh1[1][:], start=False, stop=True)
        st = per.tile([rows, 1], dt.float32, name=f"s1b_{m}")
        nc.scalar.copy(st[:], sp[:])
        s1b.append(st)
    q1b = [_acc_reduce(nc, per, sq1b[m][:], rows1b[m], f"q1br{m}") for m in range(2)]
    ps1b.release()
    ar2 = _allreduce(nc, dram, per, [s1b[0][:], s1b[1][:]], [q1b[0][:], q1b[1][:]],
                     rows1b, 181, "ar2")
    sb1b = [_bn_coeffs(nc, per, ar2[m][0][:], ar2[m][1][:], g1b[m][:], b1b[m][:],
                       rows1b[m], CNT, f"bn1b{m}") for m in range(2)]

    # x1 = prelu(bn(m1)) -> hi/lo splits + sq2, all chunk-wise
    pk2op = tc.alloc_tile_pool(name="pk2op", bufs=1)   # knn2 operands (freed w/ k2pool)
    ones53 = per.tile([53, 1], dt.float32r, name="ones53")
    nc.vector.tensor_copy(ones53[:], ones_f[:53, :])
    ones128 = per.tile([128, 1], dt.float32r, name="ones128")
    nc.vector.tensor_copy(ones128[:], ones_f[:])
    x1hi = [per.tile([r, N], dt.float32r, name=f"x1s{m}_hi") for m, r in enumerate(rows1b)]
    x1lo = [pk2op.tile([r, N], dt.bfloat16, name=f"x1s{m}_lo") for m, r in enumerate(rows1b)]
    r2h_k1 = pk2op.tile([55, N], dt.float32r, name="r2h_k1")
    psq2 = tc.alloc_tile_pool(name="psq2", bufs=1, space="PSUM")
    sqt = tc.alloc_tile_pool(name="sqt", bufs=1)
    sq2p = psq2.tile([1, N], dt.float32, name="sq2p", space="PSUM")
    for c in range(4):
        sl = slice(512 * c, 512 * (c + 1))
        first = True
        for m, rows in enumerate(rows1b):
            x1c = sqt.tile([rows, 512], dt.float32, name="x1c")
            nc.scalar.activation(x1c[:], m1[m][:, sl], AF.Prelu, bias=sb1b[m][1][:],
                                 scale=sb1b[m][0][:], alpha=0.2)
            nc.vector.tensor_copy(x1hi[m][:, sl], x1c[:])
            nc.vector.tensor_tensor(out=x1lo[m][:, sl], in0=x1c[:],
                                    in1=x1hi[m][:, sl].bitcast(dt.float32),
                                    op=ALU.subtract)
            t = sqt.tile([rows, 512], dt.float32, name="x1sqc")
            nc.vector.tensor_tensor(out=t[:], in0=x1c[:], in1=x1c[:], op=ALU.mult)
            h = sqt.tile([rows, 512], dt.float32r, name="x1sqh")
            nc.vector.tensor_copy(h[:], t[:])
            l = sqt.tile([rows, 512], dt.float32r, name="x1sql")
            nc.vector.tensor_tensor(out=l[:], in0=t[:], in1=h[:].bitcast(dt.float32),
                                    op=ALU.subtract)
            on = ones128 if rows == 128 else ones53
            nc.tensor.matmul(sq2p[:, sl], on[:], h[:], start=first, stop=False)
            first = False
            nc.tensor.matmul(sq2p[:, sl], on[:], l[:],
                             start=False, stop=(m == 1))
        sc_ = sqt.tile([1, 512], dt.float32, name="sq2c")
        nc.scalar.copy(sc_[:], sq2p[:, sl])
        sh_ = sqt.tile([1, 512], dt.float32r, name="sq2ch")
        nc.vector.tensor_copy(sh_[:], sc_[:])
        sl_ = sqt.tile([1, 512], dt.float32r, name="sq2cl")
        nc.vector.tensor_tensor(out=sl_[:], in0=sc_[:], in1=sh_[:].bitcast(dt.float32),
                                op=ALU.subtract)
        nh_ = sqt.tile([1, 512], dt.float32r, name="sq2nh")
        nc.vector.tensor_scalar_mul(nh_[:], sh_[:], -1.0)
        nc.sync.dma_start(r2h_k1[53:54, sl], nh_[:])
        nl_ = sqt.tile([1, 512], dt.float32r, name="sq2nl")
        nc.vector.tensor_scalar_mul(nl_[:], sl_[:], -1.0)
        nc.sync.dma_start(r2h_k1[54:55, sl], nl_[:])
    psq2.release()
    sqt.release()

    # knn2 operands: lhsT reuses x1hi/x1lo; only scaled rhs tiles are built
    l2h_k1 = pk2op.tile([55, N], dt.float32r, name="l2h_k1")    # x1hi 128:181 + 2 ones
    nc.vector.tensor_copy(l2h_k1[0:53, :], x1hi[1][:])
    ones2f2 = pk2op.tile([2, N], dt.float32, name="ones2f2")
    nc.vector.memset(ones2f2[:], 1.0)
    ones2n = pk2op.tile([2, N], dt.float32r, name="ones2n")
    nc.vector.tensor_copy(ones2n[:], ones2f2[:])
    nc.sync.dma_start(l2h_k1[53:55, :], ones2n[:])
    r2h_k0 = pk2op.tile([P, N], dt.float32r, name="r2h_k0")     # 2*x1hi[0]
    nc.vector.tensor_scalar_mul(r2h_k0[:], x1hi[0][:], 2.0)
    nc.vector.tensor_scalar_mul(r2h_k1[0:53, :], x1hi[1][:], 2.0)
    r2l0 = pk2op.tile([P, N], dt.bfloat16, name="r2l0")         # 2*x1lo[0]
    nc.vector.tensor_scalar_mul(r2l0[:], x1lo[0][:], 2.0)
    r2l1 = pk2op.tile([53, N], dt.bfloat16, name="r2l1")        # 2*x1lo[1]
    nc.vector.tensor_scalar_mul(r2l1[:], x1lo[1][:], 2.0)
    # bf16 copies of hi operands for the lo-correction matmuls (1 cyc/col)
    hiB0 = pk2op.tile([P, N], dt.bfloat16, name="hiB0")
    nc.vector.tensor_copy(hiB0[:], x1hi[0][:].bitcast(dt.float32))
    hiB1 = pk2op.tile([53, N], dt.bfloat16, name="hiB1")
    nc.vector.tensor_copy(hiB1[:], x1hi[1][:].bitcast(dt.float32))
    r2hB0 = pk2op.tile([P, N], dt.bfloat16, name="r2hB0")
    nc.vector.tensor_copy(r2hB0[:], r2h_k0[:].bitcast(dt.float32))
    r2hB1 = pk2op.tile([53, N], dt.bfloat16, name="r2hB1")
    nc.vector.tensor_copy(r2hB1[:], r2h_k1[0:53, :].bitcast(dt.float32))

    def mm2(pe, t, c):
        msl = slice(128 * t, 128 * (t + 1))
        csl = slice(512 * c, 512 * (c + 1))
        nc.tensor.matmul(pe[:, csl], x1hi[0][:, msl], r2h_k0[:, csl], start=True, stop=False)
        nc.tensor.matmul(pe[:, csl], l2h_k1[:, msl], r2h_k1[:, csl], start=False, stop=False)
        nc.tensor.matmul(pe[:, csl], hiB0[:, msl], r2l0[:, csl], start=False, stop=False)
        nc.tensor.matmul(pe[:, csl], hiB1[:, msl], r2l1[:, csl], start=False, stop=False)
        nc.tensor.matmul(pe[:, csl], x1lo[0][:, msl], r2hB0[:, csl], start=False, stop=False)
        nc.tensor.matmul(pe[:, csl], x1lo[1][:, msl], r2hB1[:, csl], start=False, stop=True)

    iw2 = per.tile([P, S // 16], dt.uint16, name="iw_k2")
    iw2d = per.tile([P, S // 16], dt.uint16, name="iw2d")
    knn_pass(mm2, idx2_d, "k2")
    pk2op.release()
    k2pool.release()
    ph1.release()

    # ---------------- block 2 ----------------
    # conv2b weights early so their DMA overlaps b2a / AR3
    b2bpool = tc.alloc_tile_pool(name="b2bpool", bufs=1)
    w2bT_r, w2bT_b = [], []
    for i, rows in enumerate((128, 128, 128, 46)):
        wt = wld.tile([128, 595], dt.float32, name="wf")
        nc.sync.dma_start(wt[:rows, :512], ins[f"w2bT_k{i}"])
        wr = b2bpool.tile([rows, 512], dt.float32r, name=f"w2bT_r{i}")
        nc.vector.tensor_copy(wr[:], wt[:rows, :512])
        w2bT_r.append(wr)
        wb = b2bpool.tile([rows, 512], dt.bfloat16, name=f"w2bT_b{i}")
        nc.vector.tensor_copy(wb[:], wt[:rows, :512])
        w2bT_b.append(wb)

    # U2[c, n] = sum_ci du2[ci, c] x1[ci, n]   (plain f32r)
    b2pool = tc.alloc_tile_pool(name="b2pool", bufs=1)
    w2aT_r, du2T_r = [], []
    for i, rows in enumerate((128, 53)):
        wt = wld.tile([128, 595], dt.float32, name="wf")
        nc.sync.dma_start(wt[:rows, :430], ins[f"w2aT_k{i}"])
        wr = b2pool.tile([rows, 430], dt.float32r, name=f"w2aT_r{i}")
        nc.vector.tensor_copy(wr[:], wt[:rows, :430])
        w2aT_r.append(wr)
        ddt = wld.tile([128, 595], dt.float32, name="wf")
        nc.sync.dma_start(ddt[:rows, :430], ins[f"du2T_k{i}"])
        dr = b2pool.tile([rows, 430], dt.float32r, name=f"du2T_r{i}")
        nc.vector.tensor_copy(dr[:], ddt[:rows, :430])
        du2T_r.append(dr)
    rows2a_ = (128, 128, 128, 46)
    u2 = [b2pool.tile([r, N], dt.float32, name=f"u2_{i}") for i, r in enumerate(rows2a_)]
    pu2 = tc.alloc_tile_pool(name="pu2", bufs=4, space="PSUM")
    for m, rows in enumerate(rows2a_):
        msl = slice(128 * m, 128 * m + rows)
        for cc in range(4):
            sl = slice(512 * cc, 512 * (cc + 1))
            up = pu2.tile([rows, 512], dt.float32, name="up2", space="PSUM")
            nc.tensor.matmul(up[:], du2T_r[0][:, msl], x1hi[0][:, sl], start=True, stop=False)
            nc.tensor.matmul(up[:], du2T_r[1][:, msl], x1hi[1][:, sl], start=False, stop=True)
            nc.scalar.copy(u2[m][:, sl], up[:])
    pu2.release()

    rows2a = (128, 128, 128, 46)
    y2a_d = [dram.tile([r, S], dt.bfloat16, name=f"y2a_d{i}") for i, r in enumerate(rows2a)]
    sum2a = [b2pool.tile([r, NCHUNK], dt.float32, name=f"s2a_{i}") for i, r in enumerate(rows2a)]
    sq2a = [b2pool.tile([r, NCHUNK], dt.float32, name=f"q2a_{i}") for i, r in enumerate(rows2a)]

    # gather sources: bf16 copies of x1 (bf16 conv2a avoids f32r rounding casts)
    gx0 = b2pool.tile([P, N], dt.bfloat16, name="gx0")
    nc.vector.tensor_copy(gx0[:], x1hi[0][:].bitcast(dt.float32))
    gx1 = b2pool.tile([P, N], dt.bfloat16, name="gx1")
    nc.vector.tensor_copy(gx1[:], gx0[:])
    nc.vector.tensor_copy(gx1[0:53, :], x1hi[1][:].bitcast(dt.float32))
    gxi = b2pool.tile([P, 2 * N], dt.bfloat16, name="gxi")
    _g3 = gxi[:].rearrange("p (n two) -> p n two", two=2)
    nc.vector.tensor_copy(_g3[:, :, 0], gx0[:])
    nc.vector.tensor_copy(_g3[:, :, 1], gx1[:])
    w2aT_b = []
    for i, rows in enumerate((128, 53)):
        wb = b2pool.tile([rows, 430], dt.bfloat16, name=f"w2aT_b{i}")
        nc.vector.tensor_copy(wb[:], w2aT_r[i][:].bitcast(dt.float32))
        w2aT_b.append(wb)

    b2p = tc.alloc_tile_pool(name="b2p", bufs=2, space="PSUM")
    b2s = tc.alloc_tile_pool(name="b2s", bufs=4)
    b2o = tc.alloc_tile_pool(name="b2o", bufs=4)
    gxi3 = gxi[:].rearrange("p (n two) -> p n two", two=2)
    b2q = {}
    def b2_gather(c):
        cols = slice(80 * c, 80 * (c + 1))
        flat_t = idx2_d[c].rearrange("p k -> (p k)")
        nc.sync.dma_start(iw2[0:16, cols], flat_t.rearrange("(f p) -> p f", p=16))
        nc.sync.dma_start(iw2[16:32, cols], iw2[0:16, cols])
        nc.sync.dma_start(iw2[32:64, cols], iw2[0:32, cols])
        nc.sync.dma_start(iw2[64:128, cols], iw2[0:64, cols])
        nc.vector.tensor_scalar_mul(iw2d[:, cols], iw2[:, cols], 2.0)
        nbi = b2s.tile([P, 2 * CB], dt.bfloat16, name="nbi")
        nbi3 = nbi[:].rearrange("p (n two) -> p n two", two=2)
        for g in range(4):
            nc.gpsimd.indirect_copy(
                out=nbi3[:, 320 * g:320 * (g + 1), :], data=gxi3,
                idxs=iw2d[:, 80 * c + 20 * g:80 * c + 20 * (g + 1)],
                i_know_ap_gather_is_preferred=True)
        nb0r = b2s.tile([P, CB], dt.bfloat16, name="nb0r")
        nc.vector.tensor_copy(nb0r[:], nbi3[:, :, 0])
        nb1r = b2s.tile([P, CB], dt.bfloat16, name="nb1r")
        nc.vector.tensor_copy(nb1r[:], nbi3[:, :, 1])
        b2q[c] = (nb0r, nb1r)

    b2_gather(0)
    b2_gather(1)
    for c in range(NCHUNK):
        if c + 2 < NCHUNK:
            b2_gather(c + 2)
        nb0r, nb1r = b2q.pop(c)
        for m, rows in enumerate(rows2a):
            msl = slice(128 * m, 128 * m + rows)
            yp = b2p.tile([P, CB], dt.float32, name="y2ap", space="PSUM")
            for c0, c1 in CSL:
                nc.tensor.matmul(yp[:rows, c0:c1], w2aT_b[0][:, msl],
                                 nb0r[:, c0:c1], start=True, stop=False)
                nc.tensor.matmul(yp[:rows, c0:c1], w2aT_b[1][:, msl],
                                 nb1r[0:53, c0:c1], start=False, stop=True)
            uview = u2[m][:, 128 * c:128 * (c + 1), None].to_broadcast(
                [rows, 128, KNN])
            ob = b2o.tile([rows, CB], dt.bfloat16, name="ob2")
            nc.vector.scalar_tensor_tensor(
                out=ob[:].rearrange("p (n k) -> p n k", k=KNN),
                in0=yp[:rows, :].rearrange("p (n k) -> p n k", k=KNN), scalar=1.0,
                in1=uview, op0=ALU.mult, op1=ALU.add,
                accum_out=sum2a[m][:, c:c + 1])
            nc.sync.dma_start(y2a_d[m][:, CB * c:CB * (c + 1)], ob[:])
            tr = b1t.tile([rows, CB], dt.bfloat16, name="tr")
            nc.scalar.activation(tr[:], ob[:], AF.Square,
                                 accum_out=sq2a[m][:, c:c + 1])
    b2p.release()
    b2o.release()
    b2s.release()

    s2 = [_acc_reduce(nc, per, sum2a[m][:], rows2a[m], f"s2r{m}") for m in range(4)]
    q2 = [_acc_reduce(nc, per, sq2a[m][:], rows2a[m], f"q2r{m}") for m in range(4)]
    b2pool.release()

    # block-3 weights early (LIFO: pw3 below b3p/b3l/c3t/px2r); DMA overlaps conv2b
    pw3 = tc.alloc_tile_pool(name="pw3", bufs=1)
    w3aT_r = []
    rows3a_k = (128, 53, 128, 128, 128, 128)
    for i, rows in enumerate(rows3a_k):
        wt = wld.tile([128, 595], dt.float32, name="wf")
        nc.sync.dma_start(wt[:rows, :595], ins[f"w3aT_k{i}"])
        wr = pw3.tile([rows, 595], dt.bfloat16, name=f"w3aT_r{i}")
        nc.vector.tensor_copy(wr[:], wt[:rows, :595])
        w3aT_r.append(wr)
    w3bT_r = []
    rows3b_k = (128, 128, 128, 128, 83)
    for i, rows in enumerate(rows3b_k):
        wt = wld.tile([128, 595], dt.float32, name="wf")
        nc.sync.dma_start(wt[:rows, :512], ins[f"w3bT_k{i}"])
        wr = pw3.tile([rows, 512], dt.bfloat16, name=f"w3bT_r{i}")
        nc.vector.tensor_copy(wr[:], wt[:rows, :512])
        w3bT_r.append(wr)
    pb3 = tc.alloc_tile_pool(name="pb3", bufs=1)   # y3a/y3b bf16 residency

    rows2b = (128, 128, 128, 128)
    sumh2 = [b2bpool.tile([r, NCHUNK], dt.float32, name=f"sh2_{i}") for i, r in enumerate(rows2a)]
    sq2b = [b2bpool.tile([r, NCHUNK], dt.float32, name=f"q2b_{i}") for i, r in enumerate(rows2b)]
    m2 = [b2bpool.tile([r, N], dt.bfloat16, name=f"m2_{i}") for i, r in enumerate(rows2b)]

    b3p = tc.alloc_tile_pool(name="b3p", bufs=2, space="PSUM")
    b3l = tc.alloc_tile_pool(name="b3l", bufs=3)

    c2q = {}
    def c2_load(c):
        csl = slice(CB * c, CB * (c + 1))
        grp = []
        for m, rows in enumerate(rows2a):
            ld = b3l.tile([rows, CB], dt.bfloat16, name=f"ld{m}")
            nc.sync.dma_start(ld[:], y2a_d[m][:, csl])
            grp.append(ld)
        c2q[c] = grp

    c2_load(0)
    ar3 = _allreduce(nc, dram, per, [s[:] for s in s2], [q[:] for q in q2],
                     rows2a, 430, "ar3")
    sb2a = [_bn_coeffs(nc, per, ar3[m][0][:], ar3[m][1][:], g2a[m][:], b2a[m][:],
                       rows2a[m], CNT, f"bn2a{m}") for m in range(4)]
    for c in range(NCHUNK):
        if c + 1 < NCHUNK:
            c2_load(c + 1)
        csl = slice(CB * c, CB * (c + 1))
        h2 = c2q.pop(c)
        for m, rows in enumerate(rows2a):
            ld = h2[m]
            nc.scalar.activation(ld[:], ld[:], AF.Prelu, bias=sb2a[m][1][:],
                                 scale=sb2a[m][0][:], alpha=0.2,
                                 accum_out=sumh2[m][:, c:c + 1])
        for m, rows in enumerate(rows2b):
            msl = slice(128 * m, 128 * (m + 1))
            yp = b3p.tile([P, CB], dt.float32, name="y2bp", space="PSUM")
            for c0, c1 in CSL:
                for k in range(4):
                    kr = (128, 128, 128, 46)[k]
                    nc.tensor.matmul(yp[:rows, c0:c1], w2bT_b[k][:, msl],
                                     h2[k][0:kr, c0:c1],
                                     start=(k == 0), stop=(k == 3))
            tr = b1t.tile([rows, CB], dt.bfloat16, name="tr")
            nc.scalar.activation(tr[:], yp[:rows, :], AF.Square,
                                 accum_out=sq2b[m][:, c:c + 1])
            nc.vector.tensor_reduce(out=m2[m][:, 128 * c:128 * (c + 1)],
                                    in_=yp[:rows, :].rearrange("p (n k) -> p n k", k=KNN),
                                    axis=AX.X, op=ALU.max)
    b3p.release()
    b3l.release()

    sh2 = [_acc_reduce(nc, per, sumh2[m][:], rows2a[m], f"sh2r{m}") for m in range(4)]
    sh2r = []
    for m in range(4):
        t = per.tile([rows2a[m], 4], dt.float32r, name=f"sh2rr{m}")
        nc.vector.tensor_copy(t[:], sh2[m][:].to_broadcast([rows2a[m], 4]))
        sh2r.append(t)
    ps2b = tc.alloc_tile_pool(name="ps2b", bufs=1, space="PSUM")
    s2b = []
    for m in range(4):
        msl = slice(128 * m, 128 * (m + 1))
        sp = ps2b.tile([128, 4], dt.float32, name=f"s2bp{m}", space="PSUM")
        for k in range(4):
            kr = (128, 128, 128, 46)[k]
            nc.tensor.matmul(sp[:], w2bT_r[k][:, msl], sh2r[k][0:kr, :],
                             start=(k == 0), stop=(k == 3))
        st = per.tile([128, 1], dt.float32, name=f"s2b_{m}")
        nc.scalar.copy(st[:], sp[:, 0:1])
        s2b.append(st)
    ps2b.release()
    q2b = [_acc_reduce(nc, per, sq2b[m][:], 128, f"q2br{m}") for m in range(4)]
    ar4 = _allreduce(nc, dram, per, [s[:] for s in s2b], [q[:] for q in q2b],
                     rows2b, 512, "ar4")
    sb2b = [_bn_coeffs(nc, per, ar4[m][0][:], ar4[m][1][:], g2b[m][:], b2b[m][:],
                       128, CNT, f"bn2b{m}") for m in range(4)]

    # x2 = prelu(bn(m2)) -> bf16; x1 recast to bf16 for block 3
    c3t = tc.alloc_tile_pool(name="c3t", bufs=4)
    px2r = tc.alloc_tile_pool(name="px2r", bufs=1)
    x2r = [px2r.tile([128, N], dt.bfloat16, name=f"x2r_{i}") for i in range(4)]
    for m in range(4):
        nc.scalar.activation(x2r[m][:], m2[m][:], AF.Prelu, bias=sb2b[m][1][:],
                             scale=sb2b[m][0][:], alpha=0.2)
    xb1 = [px2r.tile([r, N], dt.bfloat16, name=f"xb1_{i}")
           for i, r in enumerate((128, 53))]
    for i in range(2):
        nc.vector.tensor_copy(xb1[i][:], x1hi[i][:].bitcast(dt.float32))

    # ---------------- block 3 ----------------
    # y3a / y3b stay SBUF-resident in bf16 (post-x1 path tolerates bf16)
    xc_k = [xb1[0], xb1[1], x2r[0], x2r[1], x2r[2], x2r[3]]
    rows3a = (128, 128, 128, 128, 83)
    y3a_s = [pb3.tile([r, N], dt.bfloat16, name=f"y3a_s{i}") for i, r in enumerate(rows3a)]
    sum3a = [per.tile([r, 4], dt.float32, name=f"s3a_{i}") for i, r in enumerate(rows3a)]
    sq3a = [per.tile([r, 4], dt.float32, name=f"q3a_{i}") for i, r in enumerate(rows3a)]
    c3p = tc.alloc_tile_pool(name="c3p", bufs=4, space="PSUM")
    for c in range(4):
        csl = slice(512 * c, 512 * (c + 1))
        for m, rows in enumerate(rows3a):
            msl = slice(128 * m, 128 * m + rows)
            yp = c3p.tile([128, 512], dt.float32, name="y3ap", space="PSUM")
            for k in range(6):
                nc.tensor.matmul(yp[:rows, :], w3aT_r[k][:, msl], xc_k[k][:, csl],
                                 start=(k == 0), stop=(k == 5))
            nc.scalar.activation(y3a_s[m][:, csl], yp[:rows, :], AF.Copy,
                                 accum_out=sum3a[m][:, c:c + 1])
            tr = c3t.tile([rows, 512], dt.bfloat16, name="tr")
            nc.scalar.activation(tr[:], yp[:rows, :], AF.Square,
                                 accum_out=sq3a[m][:, c:c + 1])
    c3p.release()
    px2r.release()

    s3 = [_acc_reduce(nc, per, sum3a[m][:], rows3a[m], f"s3r{m}") for m in range(5)]
    q3 = [_acc_reduce(nc, per, sq3a[m][:], rows3a[m], f"q3r{m}") for m in range(5)]
    ar5 = _allreduce(nc, dram, per, [s[:] for s in s3], [q[:] for q in q3],
                     rows3a, 640, "ar5")
    sb3a = [_bn_coeffs(nc, per, ar5[m][0][:], ar5[m][1][:], g3a[m][:], b3a[m][:],
                       rows3a[m], CNT3, f"bn3a{m}") for m in range(5)]

    rows3b = (128, 128, 128, 128)
    y3b_s = [pb3.tile([r, N], dt.bfloat16, name=f"y3b_s{i}") for i, r in enumerate(rows3b)]
    sum3b = [per.tile([r, 4], dt.float32, name=f"s3b_{i}") for i, r in enumerate(rows3b)]
    sq3b = [per.tile([r, 4], dt.float32, name=f"q3b_{i}") for i, r in enumerate(rows3b)]
    c4p = tc.alloc_tile_pool(name="c4p", bufs=2, space="PSUM")
    h3l = tc.alloc_tile_pool(name="h3l", bufs=10)
    for c in range(4):
        csl = slice(512 * c, 512 * (c + 1))
        h3c = []
        for k, kr in enumerate(rows3a):
            ld = h3l.tile([kr, 512], dt.bfloat16, name="h3ld")
            nc.scalar.activation(ld[:], y3a_s[k][:, csl], AF.Prelu,
                                 bias=sb3a[k][1][:], scale=sb3a[k][0][:], alpha=0.2)
            h3c.append(ld)
        for m, rows in enumerate(rows3b):
            msl = slice(128 * m, 128 * (m + 1))
            yp = c4p.tile([rows, 512], dt.float32, name=f"y3bp{m}", space="PSUM")
            for k in range(5):
                nc.tensor.matmul(yp[:], w3bT_r[k][:, msl], h3c[k][:],
                                 start=(k == 0), stop=(k == 4))
            nc.scalar.activation(y3b_s[m][:, csl], yp[:], AF.Copy,
                                 accum_out=sum3b[m][:, c:c + 1])
            tr = c3t.tile([rows, 512], dt.bfloat16, name="tr")
            nc.scalar.activation(tr[:], yp[:], AF.Square,
                                 accum_out=sq3b[m][:, c:c + 1])
    c4p.release()
    h3l.release()
    c3t.release()

    s4 = [_acc_reduce(nc, per, sum3b[m][:], 128, f"s4r{m}") for m in range(4)]
    q4 = [_acc_reduce(nc, per, sq3b[m][:], 128, f"q4r{m}") for m in range(4)]
    ar6 = _allreduce(nc, dram, per, [s[:] for s in s4], [q[:] for q in q4],
                     rows3b, 512, "ar6")
    sb3b = [_bn_coeffs(nc, per, ar6[m][0][:], ar6[m][1][:], g3b[m][:], b3b[m][:],
                       128, CNT3, f"bn3b{m}") for m in range(4)]

    out_d = nc._ext_outputs["out"]
    for m in range(4):
        for c in range(4):
            csl = slice(512 * c, 512 * (c + 1))
            ot = oseg.tile([128, 512], dt.float32, name="ot")
            nc.scalar.activation(ot[:], y3b_s[m][:, csl], AF.Prelu,
                                 bias=sb3b[m][1][:], scale=sb3b[m][0][:], alpha=0.2)
            nc.sync.dma_start(out_d[128 * m:128 * (m + 1), csl], ot[:])
    pb3.release()
    pw3.release()
    b2bpool.release()
    wld.release()
    oseg.release()
    b1t.release()
    per.release()


# ------------------------------------------------------------------ host side
_CACHE = {}


def _build():
    _install_bassfix()
    nc = bass.Bass("TRN2", target_bir_lowering=False, debug=False,
                   num_devices=NCORES)
    in_specs = {
        "x": (64, N), "w1aT": (64, 152), "du1T": (64, 152),
        "w1bT_k0": (128, 181), "w1bT_k1": (24, 181),
        "w2aT_k0": (128, 430), "w2aT_k1": (53, 430),
        "du2T_k0": (128, 430), "du2T_k1": (53, 430),
        "w2bT_k0": (128, 512), "w2bT_k1": (128, 512),
        "w2bT_k2": (128, 512), "w2bT_k3": (46, 512),
        "w3aT_k0": (128, 595), "w3aT_k1": (53, 595), "w3aT_k2": (128, 595),
        "w3aT_k3": (128, 595), "w3aT_k4": (128, 595), "w3aT_k5": (128, 595),
        "w3bT_k0": (128, 512), "w3bT_k1": (128, 512), "w3bT_k2": (128, 512),
        "w3bT_k3": (128, 512), "w3bT_k4": (83, 512),
        "g1a": (152, 1), "be1a": (152, 1), "g1b": (181, 1), "be1b": (181, 1),
        "g2a": (430, 1), "be2a": (430, 1), "g2b": (512, 1), "be2b": (512, 1),
        "g3a": (595, 1), "be3a": (595, 1), "g3b": (512, 1), "be3b": (512, 1),
        "E": (32, CB),
    }
    nc._ext_inputs = {}
    for nm, shp in in_specs.items():
        nc._ext_inputs[nm] = nc.dram_tensor(nm, list(shp), dt.float32,
                                            kind="ExternalInput").ap()
    nc._ext_outputs = {
        "out": nc.dram_tensor("out", [512, N], dt.float32,
                              kind="ExternalOutput").ap()}
    with tile.TileContext(nc) as tc:
        build_kernel(nc, tc)
    return nc


def _host_inputs(x, c1w1, c1g1, c1be1, c1w2, c1g2, c1be2,
                 c2w1, c2g1, c2be1, c2w2, c2g2, c2be2,
                 c3w1, c3g1, c3be1, c3w2, c3g2, c3be2):
    f32 = np.float32
    W1 = np.asarray(c1w1, f32)            # [152, 128]
    W1a, W1b = W1[:, :64], W1[:, 64:]
    W2 = np.asarray(c2w1, f32)            # [430, 362]
    W2a, W2b = W2[:, :181], W2[:, 181:]
    ws = {
        "w1aT": np.ascontiguousarray(W1a.T),
        "du1T": np.ascontiguousarray((W1b - W1a).T),
    }
    w1bT = np.ascontiguousarray(np.asarray(c1w2, f32).T)     # [152, 181]
    ws["w1bT_k0"], ws["w1bT_k1"] = w1bT[:128], w1bT[128:]
    w2aT = np.ascontiguousarray(W2a.T)                        # [181, 430]
    du2T = np.ascontiguousarray((W2b - W2a).T)
    ws["w2aT_k0"], ws["w2aT_k1"] = w2aT[:128], w2aT[128:]
    ws["du2T_k0"], ws["du2T_k1"] = du2T[:128], du2T[128:]
    w2bT = np.ascontiguousarray(np.asarray(c2w2, f32).T)     # [430, 512]
    for i, sl in enumerate((slice(0, 128), slice(128, 256), slice(256, 384),
                            slice(384, 430))):
        ws[f"w2bT_k{i}"] = w2bT[sl]
    w3aT = np.ascontiguousarray(np.asarray(c3w1, f32).T)     # [693, 595]
    cuts = (0, 128, 181, 309, 437, 565, 693)
    for i in range(6):
        ws[f"w3aT_k{i}"] = w3aT[cuts[i]:cuts[i + 1]]
    w3bT = np.ascontiguousarray(np.asarray(c3w2, f32).T)     # [595, 512]
    for i in range(5):
        ws[f"w3bT_k{i}"] = w3bT[128 * i:min(128 * (i + 1), 595)]
    for nm, v in (("g1a", c1g1), ("be1a", c1be1), ("g1b", c1g2), ("be1b", c1be2),
                  ("g2a", c2g1), ("be2a", c2be1), ("g2b", c2g2), ("be2b", c2be2),
                  ("g3a", c3g1), ("be3a", c3be1), ("g3b", c3g2), ("be3b", c3be2)):
        ws[nm] = np.ascontiguousarray(np.asarray(v, f32).reshape(-1, 1))
    E = np.zeros((32, CB), f32)
    for r in range(32):
        E[r, KNN * r:KNN * (r + 1)] = 1.0
    ws["E"] = E
    ws = {k: np.ascontiguousarray(v, f32) for k, v in ws.items()}
    xs = np.asarray(x, f32)
    in_maps = []
    for i in range(NCORES):
        m = dict(ws)
        m["x"] = np.ascontiguousarray(xs[i])
        in_maps.append(m)
    return in_maps


def kernel(x, c1w1, c1b1, c1g1, c1be1, c1w2, c1b2, c1g2, c1be2,
           c2w1, c2b1, c2g1, c2be1, c2w2, c2b2, c2g2, c2be2,
           c3w1, c3b1, c3g1, c3be1, c3w2, c3b2, c3g2, c3be2):
    # conv biases are absorbed exactly by the following BatchNorm (shift
    # invariance), so b* inputs are unused by the device program.
    if "nc" not in _CACHE:
        _CACHE["nc"] = _build()
    nc = _CACHE["nc"]
    in_maps = _host_inputs(x, c1w1, c1g1, c1be1, c1w2, c1g2, c1be2,
                           c2w1, c2g1, c2be1, c2w2, c2g2, c2be2,
                           c3w1, c3g1, c3be1, c3w2, c3g2, c3be2)
    res = run_bass_kernel_spmd(nc, in_maps, core_ids=list(range(NCORES)))
    _CACHE["last_result"] = res
    out = np.stack([res.results[i]["out"] for i in range(NCORES)], axis=0)
    return out.astype(np.float32)



# revision 38
# speedup vs baseline: 1.1095x; 1.0035x over previous
"""DGCNN-style GCN encoder on 8 TRN2 NeuronCores (bass/tile).

Data-parallel over batch: each core owns one sample (B=8). BatchNorm batch
statistics are all-reduced across the 8 cores so results match global-batch
BN exactly. kNN top-10, neighbor gathers, and all convs/BN/activations run
on-device; the host only reshapes/replicates weights.

Precision strategy: matmuls on the x -> x1 -> knn2 path use a compensated
fp32r (hi/lo split) scheme that recovers ~fp32 accuracy at fp32r speed
(conv1b uses plain fp32). This keeps the kNN neighbor selection stable
against the f32 reference. Everything downstream of x1 (conv2a/2b, block 3)
runs in bf16 (weights + activations), which the 2e-2 tolerance absorbs.

Performance notes (vs the first working version):
- edge loops process 1280-edge chunks (128 points x 10 nbrs) instead of 320
- neighbor gathers are indirect_copy, whose cost is ~28ns per index of Q7
  dispatch; conv2a gathers both x1 row-tiles in ONE pass by interleaving
  them as bf16 pairs (d=2 gather with doubled indices), halving index count
- gathers/loads are software-pipelined one-to-two chunks ahead, and the
  next phase's first loads + weight loads are issued before each BN
  AllReduce so DMA overlaps the collective
- block-1's gather stream overlaps knn1 (per-tile iw builds + pools
  pre-allocated before the knn pass so gather t only waits on top-k tile t);
  iw2/iw2d are likewise built per-tile during knn2
- knn2's lo-correction matmuls (4 of 6) run in bf16 at 1 cyc/col; only the
  two hi*hi + sq-fold matmuls stay compensated f32r, keeping neighbor
  selection error ~5e-5 absolute
- y3a/y3b stay SBUF-resident in bf16 (no DRAM round-trip); y2a is spilled
  to DRAM in bf16 (half the traffic)
"""

import os
import sys
import json

B16 = "/nix/store/wxap7svlj45h0lfm31d1axjjnzyl6qsy-b16-bazel-unstable-cc-2026-05-04-9a3fa1f3-rt-2026-05-04-ade39e0a/lib/python3.13/site-packages"
if B16 not in sys.path:
    sys.path.insert(0, B16)
if "/opt/trn_rl_repo" not in sys.path:
    sys.path.insert(0, "/opt/trn_rl_repo")

import numpy as np
import concourse.bass as bass
import concourse.mybir as mybir
import concourse.tile as tile
from concourse.bass_utils import run_bass_kernel_spmd

dt = mybir.dt
AF = mybir.ActivationFunctionType
ALU = mybir.AluOpType
AX = mybir.AxisListType

N = 2048
KNN = 10
S = N * KNN            # 20480 edge positions
CB = 1280              # banded chunk: 128 points x 10 neighbors
NCHUNK = S // CB       # 16
CSL = [(0, 512), (512, 1024), (1024, 1280)]   # matmul col slices of a chunk
EPS = 1e-5
NCORES = 8
CNT = float(NCORES * S)      # BN count for edge tensors
CNT3 = float(NCORES * N)     # BN count for block-3 tensors


# ---------------------------------------------------------------- walrus fix
def _fix_bir_json(bir_json: bytes) -> bytes:
    """This container's walrus accepts only ONE sync-wait per instruction;
    hoist extra waits onto preceding EventSemaphore instructions."""
    m = json.loads(bir_json)
    for f in m["functions"]:
        for bb in f["blocks"]:
            out = []
            for i in bb["instructions"]:
                si = i.get("sync_info") or {}
                waits = si.get("on_wait") or []
                if len(waits) > 1:
                    for k, w in enumerate(waits[:-1]):
                        out.append({
                            "name": f"{i['name']}w{k}",
                            "engine": i["engine"],
                            "opcode": "EventSemaphore",
                            "ins": [], "outs": [],
                            "debug": i.get("debug"),
                            "sync_info": {"on_update": [], "on_wait": [w]},
                        })
                    i = dict(i)
                    si = dict(si)
                    si["on_wait"] = [waits[-1]]
                    i["sync_info"] = si
                out.append(i)
            bb["instructions"] = out
    return json.dumps(m).encode()


def _install_bassfix():
    import concourse.bass_utils as bu
    import concourse.bass2jax as b2j
    if getattr(bu, "_bassfix_installed", False):
        return
    real = bu.compile_bir_kernel

    def patched(bir_json, tmpdir, neff_name="file.neff"):
        return real(_fix_bir_json(bir_json), tmpdir, neff_name)

    bu.compile_bir_kernel = patched
    b2j.compile_bir_kernel = patched
    bu._bassfix_installed = True


# ------------------------------------------------------------- device helpers
def _round_split(nc, pool, src_ap, parts, width, name):
    """Return (hi, lo) float32r tiles for a f32 source AP [parts, width]."""
    hi = pool.tile([parts, width], dt.float32r, name=name + "_hi")
    lo = pool.tile([parts, width], dt.float32r, name=name + "_lo")
    nc.vector.tensor_copy(hi[:], src_ap)
    nc.vector.tensor_tensor(out=lo[:], in0=src_ap, in1=hi[:].bitcast(dt.float32),
                            op=ALU.subtract)
    return hi, lo


def _bn_coeffs(nc, pool, sums, sqs, g_ap, b_ap, rows, cnt, name):
    """sums/sqs: [rows,1] f32 (global). Returns (scale, bias) [rows,1] f32."""
    mean = pool.tile([rows, 1], dt.float32, name=name + "_mean")
    nc.vector.tensor_scalar_mul(mean[:], sums, 1.0 / cnt)
    e2 = pool.tile([rows, 1], dt.float32, name=name + "_e2")
    nc.vector.tensor_scalar_mul(e2[:], sqs, 1.0 / cnt)
    m2 = pool.tile([rows, 1], dt.float32, name=name + "_m2")
    nc.vector.tensor_tensor(out=m2[:], in0=mean[:], in1=mean[:], op=ALU.mult)
    var = pool.tile([rows, 1], dt.float32, name=name + "_var")
    nc.vector.tensor_tensor(out=var[:], in0=e2[:], in1=m2[:], op=ALU.subtract)
    epst = pool.tile([rows, 1], dt.float32, name=name + "_eps")
    nc.vector.memset(epst[:], float(EPS))
    std = pool.tile([rows, 1], dt.float32, name=name + "_std")
    nc.scalar.activation(std[:], var[:], AF.Sqrt, bias=epst[:])
    rstd = pool.tile([rows, 1], dt.float32, name=name + "_rstd")
    nc.vector.reciprocal(rstd[:], std[:])
    scale = pool.tile([rows, 1], dt.float32, name=name + "_scale")
    nc.vector.tensor_tensor(out=scale[:], in0=rstd[:], in1=g_ap, op=ALU.mult)
    ms = pool.tile([rows, 1], dt.float32, name=name + "_ms")
    nc.vector.tensor_tensor(out=ms[:], in0=mean[:], in1=scale[:], op=ALU.mult)
    bias = pool.tile([rows, 1], dt.float32, name=name + "_bias")
    nc.vector.tensor_tensor(out=bias[:], in0=b_ap, in1=ms[:], op=ALU.subtract)
    return scale, bias


def _allreduce(nc, dram, per, sums_list, sqs_list, rows_list, cpad, name):
    """Pack per-Mtile [rows,1] sum/sq into a DRAM [cpad,2] buffer, AllReduce
    across the 8 cores, unpack back into fresh [rows,1] tiles."""
    cin = dram.tile([cpad, 2], dt.float32, name=name + "_in")
    cout = dram.tile([cpad, 2], dt.float32, name=name + "_out")
    r0 = 0
    for s_, q_, rows in zip(sums_list, sqs_list, rows_list):
        nc.sync.dma_start(cin[r0:r0 + rows, 0:1], s_)
        nc.sync.dma_start(cin[r0:r0 + rows, 1:2], q_)
        r0 += rows
    if r0 < cpad:
        z = per.tile([cpad - r0, 2], dt.float32, name=name + "_z")
        nc.vector.memset(z[:], 0.0)
        nc.sync.dma_start(cin[r0:cpad, :], z[:])
    nc.gpsimd.collective_compute(
        "AllReduce", ALU.add, replica_groups=[list(range(NCORES))],
        ins=[cin[:].opt()], outs=[cout[:].opt()])
    outs = []
    r0 = 0
    for rows in rows_list:
        gs = per.tile([rows, 1], dt.float32, name=name + f"_gs{r0}")
        gq = per.tile([rows, 1], dt.float32, name=name + f"_gq{r0}")
        nc.sync.dma_start(gs[:], cout[r0:r0 + rows, 0:1])
        nc.sync.dma_start(gq[:], cout[r0:r0 + rows, 1:2])
        outs.append((gs, gq))
        r0 += rows
    return outs


def _acc_reduce(nc, per, acc, rows, name):
    """[rows, NCHUNK] accumulator -> [rows,1]."""
    out = per.tile([rows, 1], dt.float32, name=name)
    nc.vector.tensor_reduce(out=out[:], in_=acc, axis=AX.X, op=ALU.add)
    return out


def build_kernel(nc, tc):
    P = 128
    per = tc.alloc_tile_pool(name="persist", bufs=1)
    dram = tc.alloc_tile_pool(name="dram", bufs=1, space="DRAM")
    b1t = tc.alloc_tile_pool(name="b1t", bufs=2)
    oseg = tc.alloc_tile_pool(name="oseg", bufs=1)
    wld = tc.alloc_tile_pool(name="wld", bufs=2)
    ph1 = tc.alloc_tile_pool(name="ph1", bufs=1)

    ins = nc._ext_inputs
    x_d = ins["x"]

    # ---------------- load & prep ----------------
    tmp0 = tc.alloc_tile_pool(name="tmp0", bufs=1)      # until U1t/rhs built
    x = tmp0.tile([64, N], dt.float32, name="x")
    nc.sync.dma_start(x[:], x_d)
    xhi, xlo = _round_split(nc, tmp0, x[:], 64, N, "x")
    # stacks for compensated matmuls
    xstack = ph1.tile([P, N], dt.float32r, name="xstack")   # [xhi; xlo]
    nc.vector.tensor_copy(xstack[:64, :], xhi[:])
    nc.vector.tensor_copy(xstack[64:, :], xlo[:])
    lhsx = xstack                                             # [xhi; xlo]
    rhsx = ph1.tile([P, N], dt.float32r, name="rhsx")        # 2*[xlo; xhi]
    nc.vector.tensor_scalar_mul(rhsx[:64, :], xlo[:], 2.0)
    nc.vector.tensor_scalar_mul(rhsx[64:, :], xhi[:], 2.0)

    # sq1 = sum_c x^2 (compensated)
    xsq = tmp0.tile([64, N], dt.float32, name="xsq")
    nc.vector.tensor_tensor(out=xsq[:], in0=x[:], in1=x[:], op=ALU.mult)
    xsqhi, xsqlo = _round_split(nc, tmp0, xsq[:], 64, N, "xsq")
    ones_f = per.tile([128, 1], dt.float32, name="ones_f")
    nc.vector.memset(ones_f[:], 1.0)
    ones64 = per.tile([64, 1], dt.float32r, name="ones64")
    nc.vector.tensor_copy(ones64[:], ones_f[:64, :])
    # knn1 matmul operands (sq1 folded in chunk-wise)
    lhs_hi1 = ph1.tile([66, N], dt.float32r, name="lhs_hi1")
    nc.vector.tensor_copy(lhs_hi1[:64, :], xhi[:])
    ones2f = tmp0.tile([2, N], dt.float32, name="ones2f")
    nc.vector.memset(ones2f[:], 1.0)
    nc.vector.tensor_copy(lhs_hi1[64:66, :], ones2f[:])
    rhs_hi1 = ph1.tile([66, N], dt.float32r, name="rhs_hi1")
    nc.vector.tensor_scalar_mul(rhs_hi1[:64, :], xhi[:], 2.0)
    psq = tc.alloc_tile_pool(name="psq", bufs=1, space="PSUM")
    sq1p = psq.tile([1, N], dt.float32, name="sq1p", space="PSUM")
    for c in range(4):
        sl = slice(512 * c, 512 * (c + 1))
        nc.tensor.matmul(sq1p[:, sl], ones64[:], xsqhi[:, sl], start=True, stop=False)
        nc.tensor.matmul(sq1p[:, sl], ones64[:], xsqlo[:, sl], start=False, stop=True)
        sc_ = tmp0.tile([1, 512], dt.float32, name="sq1c")
        nc.scalar.copy(sc_[:], sq1p[:, sl])
        sh_ = tmp0.tile([1, 512], dt.float32r, name="sq1ch")
        nc.vector.tensor_copy(sh_[:], sc_[:])
        sl_ = tmp0.tile([1, 512], dt.float32r, name="sq1cl")
        nc.vector.tensor_tensor(out=sl_[:], in0=sc_[:], in1=sh_[:].bitcast(dt.float32),
                                op=ALU.subtract)
        nc.vector.tensor_scalar_mul(rhs_hi1[64:65, sl], sh_[:], -1.0)
        nl_ = tmp0.tile([1, 512], dt.float32r, name="sq1nl")
        nc.vector.tensor_scalar_mul(nl_[:], sl_[:], -1.0)
        nc.sync.dma_start(rhs_hi1[65:66, sl], nl_[:])
    psq.release()

    # conv1 weights
    w1aT = wld.tile([128, 595], dt.float32, name="wf")
    nc.sync.dma_start(w1aT[:64, :152], ins["w1aT"])
    w1aT_hi, w1aT_lo = _round_split(nc, per, w1aT[:64, :152], 64, 152, "w1aT")
    du1T = wld.tile([128, 595], dt.float32, name="wf")
    nc.sync.dma_start(du1T[:64, :152], ins["du1T"])
    du1T_hi, du1T_lo = _round_split(nc, per, du1T[:64, :152], 64, 152, "du1T")
    w1aX = per.tile([P, 152], dt.float32r, name="w1aX")      # [Wlo; Whi]
    nc.vector.tensor_copy(w1aX[:64, :], w1aT_lo[:])
    nc.vector.tensor_copy(w1aX[64:, :], w1aT_hi[:])
    du1X = per.tile([P, 152], dt.float32r, name="du1X")      # [dlo; dhi] pairs lhsx
    nc.vector.tensor_copy(du1X[:64, :], du1T_lo[:])
    nc.vector.tensor_copy(du1X[64:, :], du1T_hi[:])

    wkt1b = []  # conv1b fp32 weights, K-tiles
    for i, rows in enumerate((128, 24)):
        t = per.tile([rows, 181], dt.float32, name=f"w1bT_k{i}")
        nc.sync.dma_start(t[:], ins[f"w1bT_k{i}"])
        wkt1b.append(t)


    # U1[c, n] = sum_ci du1[ci, c] x[ci, n]   (compensated, natural layout)
    rows1a_ = (128, 24)
    u1 = [ph1.tile([r, N], dt.float32, name=f"u1_{i}") for i, r in enumerate(rows1a_)]
    pu = tc.alloc_tile_pool(name="pu", bufs=4, space="PSUM")
    for m, rows in enumerate(rows1a_):
        msl = slice(128 * m, 128 * m + rows)
        for cc in range(4):
            sl = slice(512 * cc, 512 * (cc + 1))
            up = pu.tile([rows, 512], dt.float32, name="up", space="PSUM")
            nc.tensor.matmul(up[:], du1T_hi[:, msl], xhi[:, sl], start=True, stop=False)
            nc.tensor.matmul(up[:], du1X[:, msl], xstack[:, sl], start=False, stop=True)
            nc.scalar.copy(u1[m][:, sl], up[:])
    pu.release()
    tmp0.release()

    # BN params
    def load_vec(nm, rows_list):
        tiles = []
        r0 = 0
        for i, rows in enumerate(rows_list):
            t = per.tile([rows, 1], dt.float32, name=f"{nm}_{i}")
            nc.sync.dma_start(t[:], ins[nm][r0:r0 + rows, :])
            tiles.append(t)
            r0 += rows
        return tiles

    g1a = load_vec("g1a", (128, 24)); b1a = load_vec("be1a", (128, 24))
    g1b = load_vec("g1b", (128, 53)); b1b = load_vec("be1b", (128, 53))
    g2a = load_vec("g2a", (128, 128, 128, 46)); b2a = load_vec("be2a", (128, 128, 128, 46))
    g2b = load_vec("g2b", (128,) * 4); b2b = load_vec("be2b", (128,) * 4)
    g3a = load_vec("g3a", (128,) * 4 + (83,)); b3a = load_vec("be3a", (128,) * 4 + (83,))
    g3b = load_vec("g3b", (128,) * 4); b3b = load_vec("be3b", (128,) * 4)

    # ---------------- kNN (shared machinery) ----------------
    idx1_d = dram.tile([16, P, KNN], dt.uint16, name="idx1_d")
    idx2_d = dram.tile([16, P, KNN], dt.uint16, name="idx2_d")

    def knn_pass(mm_emit, idx_dram, tag, after_tile=None, kp_bufs=2):
        """mm_emit(psum_slice, c) emits matmuls for columns 512c..512c+512."""
        kp = tc.alloc_tile_pool(name=f"knnp_{tag}", bufs=kp_bufs, space="PSUM")
        ks = tc.alloc_tile_pool(name=f"knns_{tag}", bufs=2)
        for t in range(16):
            pe = kp.tile([P, N], dt.float32, name="pe", space="PSUM")
            for c in range(4):
                mm_emit(pe, t, c)
            q = ks.tile([P, N], dt.float32, name="q")
            nc.scalar.copy(q[:], pe[:])
            v8 = ks.tile([P, 8], dt.float32, name="v8")
            i8 = ks.tile([P, 8], dt.uint16, name="i8")
            nc.vector.max(out=v8[:], in_=q[:])
            nc.vector.max_index(out=i8[:], in_max=v8[:], in_values=q[:])
            nc.vector.match_replace(out=q[:], in_to_replace=v8[:], in_values=q[:],
                                    imm_value=-1e30)
            v8b = ks.tile([P, 8], dt.float32, name="v8b")
            i8b = ks.tile([P, 8], dt.uint16, name="i8b")
            nc.vector.max(out=v8b[:], in_=q[:])
            nc.vector.max_index(out=i8b[:], in_max=v8b[:], in_values=q[:])
            idx = ks.tile([P, KNN], dt.uint16, name="idx")
            nc.vector.tensor_copy(idx[:, 0:7], i8[:, 1:8])
            nc.vector.tensor_copy(idx[:, 7:10], i8b[:, 0:3])
            nc.sync.dma_start(idx_dram[t], idx[:])
            if after_tile is not None:
                after_tile(t)
        kp.release()
        ks.release()

    def build_wrapped(idx_dram, tag):
        """DRAM flat idx (point-major) -> wrapped [128, S/16] uint16."""
        iw = per.tile([P, S // 16], dt.uint16, name=f"iw_{tag}")
        flat = idx_dram[:].rearrange("a p k -> (a p k)")
        nc.sync.dma_start(iw[0:16, :], flat.rearrange("(f p) -> p f", p=16))
        for g in range(1, 8):
            nc.sync.dma_start(iw[16 * g:16 * (g + 1), :], iw[0:16, :])
        return iw

    # knn1 matmuls
    def mm1(pe, t, c):
        msl = slice(128 * t, 128 * (t + 1))
        csl = slice(512 * c, 512 * (c + 1))
        nc.tensor.matmul(pe[:, csl], lhs_hi1[:, msl], rhs_hi1[:, csl],
                         start=True, stop=False)
        nc.tensor.matmul(pe[:, csl], lhsx[:, msl], rhsx[:, csl],
                         start=False, stop=True)

    # ---------------- block 1, interleaved with knn1 ----------------
    # y1a[(n,j)] = W1a @ (nb - cen) + W1b @ cen ; stats; keep for BN.
    # Each knn1 tile's top-k feeds its 4 stats chunks immediately, so the
    # gather-paced loop overlaps the DVE top-k of later tiles.
    y1a_d = [dram.tile([P, S], dt.float32, name="y1a_d0"),
             dram.tile([24, S], dt.float32, name="y1a_d1")]
    rows1a = (128, 24)
    sum1a = [ph1.tile([r, NCHUNK], dt.float32, name=f"s1a_{i}") for i, r in enumerate(rows1a)]
    sq1a = [ph1.tile([r, NCHUNK], dt.float32, name=f"q1a_{i}") for i, r in enumerate(rows1a)]
    # b1 pools pre-allocated so the gather stream overlaps knn1; iw1 built
    # per-tile so gather t only depends on top-k tile t
    iw1 = per.tile([P, S // 16], dt.uint16, name="iw_k1")
    b1p = tc.alloc_tile_pool(name="b1p", bufs=1, space="PSUM")
    b1s = tc.alloc_tile_pool(name="b1s", bufs=3)
    knn_pass(mm1, idx1_d, "k1", kp_bufs=1)

    b1q = {}
    def b1_gather(t):
        cols = slice(80 * t, 80 * (t + 1))
        flat_t = idx1_d[t].rearrange("p k -> (p k)")
        nc.sync.dma_start(iw1[0:16, cols], flat_t.rearrange("(f p) -> p f", p=16))
        nc.sync.dma_start(iw1[16:32, cols], iw1[0:16, cols])
        nc.sync.dma_start(iw1[32:64, cols], iw1[0:32, cols])
        nc.sync.dma_start(iw1[64:128, cols], iw1[0:64, cols])
        nbr = b1s.tile([P, CB], dt.float32, name="nbr")
        for g in range(2):
            nc.gpsimd.indirect_copy(out=nbr[:, 640 * g:640 * (g + 1)],
                                    data=xstack[:].bitcast(dt.float32),
                                    idxs=iw1[:, 80 * t + 40 * g:80 * t + 40 * (g + 1)],
                                    i_know_ap_gather_is_preferred=True)
        nbxt = b1s.tile([P, CB], dt.float32r, name="nbxt")
        nc.scalar.copy(nbxt[:], nbr[:])
        b1q[t] = nbxt

    b1_gather(0)
    for t in range(NCHUNK):
        if t + 1 < NCHUNK:
            b1_gather(t + 1)
        nbx = b1q.pop(t)[:]
        csl = slice(CB * t, CB * (t + 1))
        for m, rows in enumerate(rows1a):
            msl = slice(128 * m, 128 * m + rows)
            yp = b1p.tile([P, CB], dt.float32, name="yp", space="PSUM")
            for c0, c1 in CSL:
                nc.tensor.matmul(yp[:rows, c0:c1], w1aT_hi[:, msl],
                                 nbx[0:64, c0:c1], start=True, stop=False)
                nc.tensor.matmul(yp[:rows, c0:c1], w1aX[:, msl],
                                 nbx[:, c0:c1], start=False, stop=True)
            uview = u1[m][:, 128 * t:128 * (t + 1), None].to_broadcast(
                [rows, 128, KNN])
            ob = b1s.tile([rows, CB], dt.float32, name="ob1")
            nc.vector.scalar_tensor_tensor(
                out=ob[:].rearrange("p (n k) -> p n k", k=KNN),
                in0=yp[:rows, :].rearrange("p (n k) -> p n k", k=KNN), scalar=1.0,
                in1=uview, op0=ALU.mult, op1=ALU.add,
                accum_out=sum1a[m][:, t:t + 1])
            nc.sync.dma_start(y1a_d[m][:, csl], ob[:])
            tr = b1t.tile([rows, CB], dt.bfloat16, name="tr")
            nc.scalar.activation(tr[:], ob[:], AF.Square,
                                 accum_out=sq1a[m][:, t:t + 1])
    b1s.release()
    b1p.release()
    k2pool = tc.alloc_tile_pool(name="k2pool", bufs=1)

    s1 = [_acc_reduce(nc, per, sum1a[m][:], rows1a[m], f"s1r{m}") for m in range(2)]
    q1 = [_acc_reduce(nc, per, sq1a[m][:], rows1a[m], f"q1r{m}") for m in range(2)]

    # h1 load prefetch issued before the AllReduce so DMA overlaps it
    sumh1 = [ph1.tile([r, NCHUNK], dt.float32, name=f"sh1_{i}") for i, r in enumerate(rows1a)]
    rows1b = (128, 53)
    m1 = [k2pool.tile([r, N], dt.float32, name=f"m1_{i}") for i, r in enumerate(rows1b)]
    sq1b = [ph1.tile([r, NCHUNK], dt.float32, name=f"q1b_{i}") for i, r in enumerate(rows1b)]
    b2pp = tc.alloc_tile_pool(name="b2pp", bufs=2, space="PSUM")
    h1l = tc.alloc_tile_pool(name="h1l", bufs=2)

    h1q = {}
    def h1_load(c):
        csl = slice(CB * c, CB * (c + 1))
        pair = []
        for m, rows in enumerate(rows1a):
            ld = h1l.tile([rows, CB], dt.float32, name=f"h1ld{m}")
            nc.sync.dma_start(ld[:], y1a_d[m][:, csl])
            pair.append(ld)
        h1q[c] = pair

    h1_load(0)
    ar1 = _allreduce(nc, dram, per, [s1[0][:], s1[1][:]], [q1[0][:], q1[1][:]],
                     rows1a, 152, "ar1")
    sb1a = [_bn_coeffs(nc, per, ar1[m][0][:], ar1[m][1][:], g1a[m][:], b1a[m][:],
                       rows1a[m], CNT, f"bn1a{m}") for m in range(2)]
    for c in range(NCHUNK):
        if c + 1 < NCHUNK:
            h1_load(c + 1)
        csl = slice(CB * c, CB * (c + 1))
        h1c = h1q.pop(c)
        for m, rows in enumerate(rows1a):
            ld = h1c[m]
            nc.scalar.activation(ld[:], ld[:], AF.Prelu,
                                 bias=sb1a[m][1][:], scale=sb1a[m][0][:], alpha=0.2,
                                 accum_out=sumh1[m][:, c:c + 1])
        for m, rows in enumerate(rows1b):
            msl = slice(128 * m, 128 * m + rows)
            yp = b2pp.tile([P, CB], dt.float32, name="y1bp", space="PSUM")
            for c0, c1 in CSL:
                nc.tensor.matmul(yp[:rows, c0:c1], wkt1b[0][:, msl],
                                 h1c[0][:, c0:c1], start=True, stop=False)
                nc.tensor.matmul(yp[:rows, c0:c1], wkt1b[1][:, msl],
                                 h1c[1][:, c0:c1], start=False, stop=True)
            tr = b1t.tile([rows, CB], dt.bfloat16, name="tr")
            nc.scalar.activation(tr[:], yp[:rows, :], AF.Square,
                                 accum_out=sq1b[m][:, c:c + 1])
            nc.vector.tensor_reduce(out=m1[m][:, 128 * c:128 * (c + 1)],
                                    in_=yp[:rows, :].rearrange("p (n k) -> p n k", k=KNN),
                                    axis=AX.X, op=ALU.max)
    b2pp.release()
    h1l.release()

    # sum(y1b) = W1b' @ sum(h1)
    sh1 = [_acc_reduce(nc, per, sumh1[m][:], rows1a[m], f"sh1r{m}") for m in range(2)]
    ps1b = tc.alloc_tile_pool(name="ps1b", bufs=1, space="PSUM")
    s1b = []
    for m, rows in enumerate(rows1b):
        msl = slice(128 * m, 128 * m + rows)
        sp = ps1b.tile([rows, 1], dt.float32, name=f"s1bp{m}", space="PSUM")
        nc.tensor.matmul(sp[:], wkt1b[0][:, msl], sh1[0][:], start=True, stop=False)
        nc.tensor.matmul(sp[:], wkt1b[1][:, msl], sh1[1][:], start=False, stop=True)
        st = per.tile([rows, 1], dt.float32, name=f"s1b_{m}")
        nc.scalar.copy(st[:], sp[:])
        s1b.append(st)
    q1b = [_acc_reduce(nc, per, sq1b[m][:], rows1b[m], f"q1br{m}") for m in range(2)]
    ps1b.release()
    ar2 = _allreduce(nc, dram, per, [s1b[0][:], s1b[1][:]], [q1b[0][:], q1b[1][:]],
                     rows1b, 181, "ar2")
    sb1b = [_bn_coeffs(nc, per, ar2[m][0][:], ar2[m][1][:], g1b[m][:], b1b[m][:],
                       rows1b[m], CNT, f"bn1b{m}") for m in range(2)]

    # x1 = prelu(bn(m1)) -> hi/lo splits + sq2, all chunk-wise
    pk2op = tc.alloc_tile_pool(name="pk2op", bufs=1)   # knn2 operands (freed w/ k2pool)
    ones53 = per.tile([53, 1], dt.float32r, name="ones53")
    nc.vector.tensor_copy(ones53[:], ones_f[:53, :])
    ones128 = per.tile([128, 1], dt.float32r, name="ones128")
    nc.vector.tensor_copy(ones128[:], ones_f[:])
    x1hi = [per.tile([r, N], dt.float32r, name=f"x1s{m}_hi") for m, r in enumerate(rows1b)]
    x1lo = [pk2op.tile([r, N], dt.bfloat16, name=f"x1s{m}_lo") for m, r in enumerate(rows1b)]
    r2h_k1 = pk2op.tile([55, N], dt.float32r, name="r2h_k1")
    psq2 = tc.alloc_tile_pool(name="psq2", bufs=1, space="PSUM")
    sqt = tc.alloc_tile_pool(name="sqt", bufs=1)
    sq2p = psq2.tile([1, N], dt.float32, name="sq2p", space="PSUM")
    for c in range(4):
        sl = slice(512 * c, 512 * (c + 1))
        first = True
        for m, rows in enumerate(rows1b):
            x1c = sqt.tile([rows, 512], dt.float32, name="x1c")
            nc.scalar.activation(x1c[:], m1[m][:, sl], AF.Prelu, bias=sb1b[m][1][:],
                                 scale=sb1b[m][0][:], alpha=0.2)
            nc.vector.tensor_copy(x1hi[m][:, sl], x1c[:])
            nc.vector.tensor_tensor(out=x1lo[m][:, sl], in0=x1c[:],
                                    in1=x1hi[m][:, sl].bitcast(dt.float32),
                                    op=ALU.subtract)
            t = sqt.tile([rows, 512], dt.float32, name="x1sqc")
            nc.vector.tensor_tensor(out=t[:], in0=x1c[:], in1=x1c[:], op=ALU.mult)
            h = sqt.tile([rows, 512], dt.float32r, name="x1sqh")
            nc.vector.tensor_copy(h[:], t[:])
            l = sqt.tile([rows, 512], dt.float32r, name="x1sql")
            nc.vector.tensor_tensor(out=l[:], in0=t[:], in1=h[:].bitcast(dt.float32),
                                    op=ALU.subtract)
            on = ones128 if rows == 128 else ones53
            nc.tensor.matmul(sq2p[:, sl], on[:], h[:], start=first, stop=False)
            first = False
            nc.tensor.matmul(sq2p[:, sl], on[:], l[:],
                             start=False, stop=(m == 1))
        sc_ = sqt.tile([1, 512], dt.float32, name="sq2c")
        nc.scalar.copy(sc_[:], sq2p[:, sl])
        sh_ = sqt.tile([1, 512], dt.float32r, name="sq2ch")
        nc.vector.tensor_copy(sh_[:], sc_[:])
        sl_ = sqt.tile([1, 512], dt.float32r, name="sq2cl")
        nc.vector.tensor_tensor(out=sl_[:], in0=sc_[:], in1=sh_[:].bitcast(dt.float32),
                                op=ALU.subtract)
        nh_ = sqt.tile([1, 512], dt.float32r, name="sq2nh")
        nc.vector.tensor_scalar_mul(nh_[:], sh_[:], -1.0)
        nc.sync.dma_start(r2h_k1[53:54, sl], nh_[:])
        nl_ = sqt.tile([1, 512], dt.float32r, name="sq2nl")
        nc.vector.tensor_scalar_mul(nl_[:], sl_[:], -1.0)
        nc.sync.dma_start(r2h_k1[54:55, sl], nl_[:])
    psq2.release()
    sqt.release()

    # knn2 operands: lhsT reuses x1hi/x1lo; only scaled rhs tiles are built
    l2h_k1 = pk2op.tile([55, N], dt.float32r, name="l2h_k1")    # x1hi 128:181 + 2 ones
    nc.vector.tensor_copy(l2h_k1[0:53, :], x1hi[1][:])
    ones2f2 = pk2op.tile([2, N], dt.float32, name="ones2f2")
    nc.vector.memset(ones2f2[:], 1.0)
    ones2n = pk2op.tile([2, N], dt.float32r, name="ones2n")
    nc.vector.tensor_copy(ones2n[:], ones2f2[:])
    nc.sync.dma_start(l2h_k1[53:55, :], ones2n[:])
    r2h_k0 = pk2op.tile([P, N], dt.float32r, name="r2h_k0")     # 2*x1hi[0]
    nc.vector.tensor_scalar_mul(r2h_k0[:], x1hi[0][:], 2.0)
    nc.vector.tensor_scalar_mul(r2h_k1[0:53, :], x1hi[1][:], 2.0)
    r2l0 = pk2op.tile([P, N], dt.bfloat16, name="r2l0")         # 2*x1lo[0]
    nc.vector.tensor_scalar_mul(r2l0[:], x1lo[0][:], 2.0)
    r2l1 = pk2op.tile([53, N], dt.bfloat16, name="r2l1")        # 2*x1lo[1]
    nc.vector.tensor_scalar_mul(r2l1[:], x1lo[1][:], 2.0)
    # bf16 copies of hi operands for the lo-correction matmuls (1 cyc/col)
    hiB0 = pk2op.tile([P, N], dt.bfloat16, name="hiB0")
    nc.vector.tensor_copy(hiB0[:], x1hi[0][:].bitcast(dt.float32))
    hiB1 = pk2op.tile([53, N], dt.bfloat16, name="hiB1")
    nc.vector.tensor_copy(hiB1[:], x1hi[1][:].bitcast(dt.float32))
    r2hB0 = pk2op.tile([P, N], dt.bfloat16, name="r2hB0")
    nc.vector.tensor_copy(r2hB0[:], r2h_k0[:].bitcast(dt.float32))
    r2hB1 = pk2op.tile([53, N], dt.bfloat16, name="r2hB1")
    nc.vector.tensor_copy(r2hB1[:], r2h_k1[0:53, :].bitcast(dt.float32))

    def mm2(pe, t, c):
        msl = slice(128 * t, 128 * (t + 1))
        csl = slice(512 * c, 512 * (c + 1))
        nc.tensor.matmul(pe[:, csl], x1hi[0][:, msl], r2h_k0[:, csl], start=True, stop=False)
        nc.tensor.matmul(pe[:, csl], l2h_k1[:, msl], r2h_k1[:, csl], start=False, stop=False)
        nc.tensor.matmul(pe[:, csl], hiB0[:, msl], r2l0[:, csl], start=False, stop=False)
        nc.tensor.matmul(pe[:, csl], hiB1[:, msl], r2l1[:, csl], start=False, stop=False)
        nc.tensor.matmul(pe[:, csl], x1lo[0][:, msl], r2hB0[:, csl], start=False, stop=False)
        nc.tensor.matmul(pe[:, csl], x1lo[1][:, msl], r2hB1[:, csl], start=False, stop=True)

    iw2 = per.tile([P, S // 16], dt.uint16, name="iw_k2")
    iw2d = per.tile([P, S // 16], dt.uint16, name="iw2d")
    knn_pass(mm2, idx2_d, "k2")
    pk2op.release()
    k2pool.release()
    ph1.release()

    # ---------------- block 2 ----------------
    # gather pools first: they reuse ph1/k2pool bytes whose users finished
    # before knn2, so the b2a gather stream can overlap knn2
    b2g = tc.alloc_tile_pool(name="b2g", bufs=1)
    gx0 = b2g.tile([P, N], dt.bfloat16, name="gx0")
    nc.vector.tensor_copy(gx0[:], x1hi[0][:].bitcast(dt.float32))
    gx1 = b2g.tile([P, N], dt.bfloat16, name="gx1")
    nc.vector.tensor_copy(gx1[:], gx0[:])
    nc.vector.tensor_copy(gx1[0:53, :], x1hi[1][:].bitcast(dt.float32))
    gxi = b2g.tile([P, 2 * N], dt.bfloat16, name="gxi")
    _g3 = gxi[:].rearrange("p (n two) -> p n two", two=2)
    nc.vector.tensor_copy(_g3[:, :, 0], gx0[:])
    nc.vector.tensor_copy(_g3[:, :, 1], gx1[:])
    b2s = tc.alloc_tile_pool(name="b2s", bufs=3)

    # conv2b weights early so their DMA overlaps b2a / AR3
    b2bpool = tc.alloc_tile_pool(name="b2bpool", bufs=1)
    w2bT_r, w2bT_b = [], []
    for i, rows in enumerate((128, 128, 128, 46)):
        wt = wld.tile([128, 595], dt.float32, name="wf")
        nc.sync.dma_start(wt[:rows, :512], ins[f"w2bT_k{i}"])
        wr = b2bpool.tile([rows, 512], dt.float32r, name=f"w2bT_r{i}")
        nc.vector.tensor_copy(wr[:], wt[:rows, :512])
        w2bT_r.append(wr)
        wb = b2bpool.tile([rows, 512], dt.bfloat16, name=f"w2bT_b{i}")
        nc.vector.tensor_copy(wb[:], wt[:rows, :512])
        w2bT_b.append(wb)

    # U2[c, n] = sum_ci du2[ci, c] x1[ci, n]   (plain f32r)
    b2pool = tc.alloc_tile_pool(name="b2pool", bufs=1)
    w2aT_r, du2T_r = [], []
    for i, rows in enumerate((128, 53)):
        wt = wld.tile([128, 595], dt.float32, name="wf")
        nc.sync.dma_start(wt[:rows, :430], ins[f"w2aT_k{i}"])
        wr = b2pool.tile([rows, 430], dt.float32r, name=f"w2aT_r{i}")
        nc.vector.tensor_copy(wr[:], wt[:rows, :430])
        w2aT_r.append(wr)
        ddt = wld.tile([128, 595], dt.float32, name="wf")
        nc.sync.dma_start(ddt[:rows, :430], ins[f"du2T_k{i}"])
        dr = b2pool.tile([rows, 430], dt.float32r, name=f"du2T_r{i}")
        nc.vector.tensor_copy(dr[:], ddt[:rows, :430])
        du2T_r.append(dr)
    rows2a_ = (128, 128, 128, 46)
    u2 = [b2pool.tile([r, N], dt.float32, name=f"u2_{i}") for i, r in enumerate(rows2a_)]
    pu2 = tc.alloc_tile_pool(name="pu2", bufs=4, space="PSUM")
    for m, rows in enumerate(rows2a_):
        msl = slice(128 * m, 128 * m + rows)
        for cc in range(4):
            sl = slice(512 * cc, 512 * (cc + 1))
            up = pu2.tile([rows, 512], dt.float32, name="up2", space="PSUM")
            nc.tensor.matmul(up[:], du2T_r[0][:, msl], x1hi[0][:, sl], start=True, stop=False)
            nc.tensor.matmul(up[:], du2T_r[1][:, msl], x1hi[1][:, sl], start=False, stop=True)
            nc.scalar.copy(u2[m][:, sl], up[:])
    pu2.release()

    rows2a = (128, 128, 128, 46)
    y2a_d = [dram.tile([r, S], dt.bfloat16, name=f"y2a_d{i}") for i, r in enumerate(rows2a)]
    sum2a = [b2pool.tile([r, NCHUNK], dt.float32, name=f"s2a_{i}") for i, r in enumerate(rows2a)]
    sq2a = [b2pool.tile([r, NCHUNK], dt.float32, name=f"q2a_{i}") for i, r in enumerate(rows2a)]

    w2aT_b = []
    for i, rows in enumerate((128, 53)):
        wb = b2pool.tile([rows, 430], dt.bfloat16, name=f"w2aT_b{i}")
        nc.vector.tensor_copy(wb[:], w2aT_r[i][:].bitcast(dt.float32))
        w2aT_b.append(wb)

    b2p = tc.alloc_tile_pool(name="b2p", bufs=2, space="PSUM")
    b2o = tc.alloc_tile_pool(name="b2o", bufs=4)
    gxi3 = gxi[:].rearrange("p (n two) -> p n two", two=2)
    b2q = {}
    def b2_gather(c):
        cols = slice(80 * c, 80 * (c + 1))
        flat_t = idx2_d[c].rearrange("p k -> (p k)")
        nc.sync.dma_start(iw2[0:16, cols], flat_t.rearrange("(f p) -> p f", p=16))
        nc.sync.dma_start(iw2[16:32, cols], iw2[0:16, cols])
        nc.sync.dma_start(iw2[32:64, cols], iw2[0:32, cols])
        nc.sync.dma_start(iw2[64:128, cols], iw2[0:64, cols])
        nc.vector.tensor_scalar_mul(iw2d[:, cols], iw2[:, cols], 2.0)
        nbi = b2s.tile([P, 2 * CB], dt.bfloat16, name="nbi")
        nbi3 = nbi[:].rearrange("p (n two) -> p n two", two=2)
        for g in range(4):
            nc.gpsimd.indirect_copy(
                out=nbi3[:, 320 * g:320 * (g + 1), :], data=gxi3,
                idxs=iw2d[:, 80 * c + 20 * g:80 * c + 20 * (g + 1)],
                i_know_ap_gather_is_preferred=True)
        nb0r = b2s.tile([P, CB], dt.bfloat16, name="nb0r")
        nc.vector.tensor_copy(nb0r[:], nbi3[:, :, 0])
        nb1r = b2s.tile([P, CB], dt.bfloat16, name="nb1r")
        nc.vector.tensor_copy(nb1r[:], nbi3[:, :, 1])
        b2q[c] = (nb0r, nb1r)

    for _pc in range(3):
        b2_gather(_pc)
    for c in range(NCHUNK):
        if c + 3 < NCHUNK:
            b2_gather(c + 3)
        nb0r, nb1r = b2q.pop(c)
        for m, rows in enumerate(rows2a):
            msl = slice(128 * m, 128 * m + rows)
            yp = b2p.tile([P, CB], dt.float32, name="y2ap", space="PSUM")
            for c0, c1 in CSL:
                nc.tensor.matmul(yp[:rows, c0:c1], w2aT_b[0][:, msl],
                                 nb0r[:, c0:c1], start=True, stop=False)
                nc.tensor.matmul(yp[:rows, c0:c1], w2aT_b[1][:, msl],
                                 nb1r[0:53, c0:c1], start=False, stop=True)
            uview = u2[m][:, 128 * c:128 * (c + 1), None].to_broadcast(
                [rows, 128, KNN])
            ob = b2o.tile([rows, CB], dt.bfloat16, name="ob2")
            nc.vector.scalar_tensor_tensor(
                out=ob[:].rearrange("p (n k) -> p n k", k=KNN),
                in0=yp[:rows, :].rearrange("p (n k) -> p n k", k=KNN), scalar=1.0,
                in1=uview, op0=ALU.mult, op1=ALU.add,
                accum_out=sum2a[m][:, c:c + 1])
            nc.sync.dma_start(y2a_d[m][:, CB * c:CB * (c + 1)], ob[:])
            tr = b1t.tile([rows, CB], dt.bfloat16, name="tr")
            nc.scalar.activation(tr[:], ob[:], AF.Square,
                                 accum_out=sq2a[m][:, c:c + 1])
    b2p.release()
    b2o.release()

    s2 = [_acc_reduce(nc, per, sum2a[m][:], rows2a[m], f"s2r{m}") for m in range(4)]
    q2 = [_acc_reduce(nc, per, sq2a[m][:], rows2a[m], f"q2r{m}") for m in range(4)]
    b2pool.release()

    # block-3 weights early (LIFO: pw3 below b3p/b3l/c3t/px2r); DMA overlaps conv2b
    pw3 = tc.alloc_tile_pool(name="pw3", bufs=1)
    w3aT_r = []
    rows3a_k = (128, 53, 128, 128, 128, 128)
    for i, rows in enumerate(rows3a_k):
        wt = wld.tile([128, 595], dt.float32, name="wf")
        nc.sync.dma_start(wt[:rows, :595], ins[f"w3aT_k{i}"])
        wr = pw3.tile([rows, 595], dt.bfloat16, name=f"w3aT_r{i}")
        nc.vector.tensor_copy(wr[:], wt[:rows, :595])
        w3aT_r.append(wr)
    w3bT_r = []
    rows3b_k = (128, 128, 128, 128, 83)
    for i, rows in enumerate(rows3b_k):
        wt = wld.tile([128, 595], dt.float32, name="wf")
        nc.sync.dma_start(wt[:rows, :512], ins[f"w3bT_k{i}"])
        wr = pw3.tile([rows, 512], dt.bfloat16, name=f"w3bT_r{i}")
        nc.vector.tensor_copy(wr[:], wt[:rows, :512])
        w3bT_r.append(wr)
    pb3 = tc.alloc_tile_pool(name="pb3", bufs=1)   # y3a/y3b bf16 residency

    rows2b = (128, 128, 128, 128)
    sumh2 = [b2bpool.tile([r, NCHUNK], dt.float32, name=f"sh2_{i}") for i, r in enumerate(rows2a)]
    sq2b = [b2bpool.tile([r, NCHUNK], dt.float32, name=f"q2b_{i}") for i, r in enumerate(rows2b)]
    m2 = [b2bpool.tile([r, N], dt.bfloat16, name=f"m2_{i}") for i, r in enumerate(rows2b)]

    b3p = tc.alloc_tile_pool(name="b3p", bufs=2, space="PSUM")
    b3l = tc.alloc_tile_pool(name="b3l", bufs=2)

    c2q = {}
    def c2_load(c):
        csl = slice(CB * c, CB * (c + 1))
        grp = []
        for m, rows in enumerate(rows2a):
            ld = b3l.tile([rows, CB], dt.bfloat16, name=f"ld{m}")
            nc.sync.dma_start(ld[:], y2a_d[m][:, csl])
            grp.append(ld)
        c2q[c] = grp

    c2_load(0)
    ar3 = _allreduce(nc, dram, per, [s[:] for s in s2], [q[:] for q in q2],
                     rows2a, 430, "ar3")
    sb2a = [_bn_coeffs(nc, per, ar3[m][0][:], ar3[m][1][:], g2a[m][:], b2a[m][:],
                       rows2a[m], CNT, f"bn2a{m}") for m in range(4)]
    for c in range(NCHUNK):
        if c + 1 < NCHUNK:
            c2_load(c + 1)
        csl = slice(CB * c, CB * (c + 1))
        h2 = c2q.pop(c)
        for m, rows in enumerate(rows2a):
            ld = h2[m]
            nc.scalar.activation(ld[:], ld[:], AF.Prelu, bias=sb2a[m][1][:],
                                 scale=sb2a[m][0][:], alpha=0.2,
                                 accum_out=sumh2[m][:, c:c + 1])
        for m, rows in enumerate(rows2b):
            msl = slice(128 * m, 128 * (m + 1))
            yp = b3p.tile([P, CB], dt.float32, name="y2bp", space="PSUM")
            for c0, c1 in CSL:
                for k in range(4):
                    kr = (128, 128, 128, 46)[k]
                    nc.tensor.matmul(yp[:rows, c0:c1], w2bT_b[k][:, msl],
                                     h2[k][0:kr, c0:c1],
                                     start=(k == 0), stop=(k == 3))
            tr = b1t.tile([rows, CB], dt.bfloat16, name="tr")
            nc.scalar.activation(tr[:], yp[:rows, :], AF.Square,
                                 accum_out=sq2b[m][:, c:c + 1])
            nc.vector.tensor_reduce(out=m2[m][:, 128 * c:128 * (c + 1)],
                                    in_=yp[:rows, :].rearrange("p (n k) -> p n k", k=KNN),
                                    axis=AX.X, op=ALU.max)
    b3p.release()
    b3l.release()

    sh2 = [_acc_reduce(nc, per, sumh2[m][:], rows2a[m], f"sh2r{m}") for m in range(4)]
    sh2r = []
    for m in range(4):
        t = per.tile([rows2a[m], 4], dt.float32r, name=f"sh2rr{m}")
        nc.vector.tensor_copy(t[:], sh2[m][:].to_broadcast([rows2a[m], 4]))
        sh2r.append(t)
    ps2b = tc.alloc_tile_pool(name="ps2b", bufs=1, space="PSUM")
    s2b = []
    for m in range(4):
        msl = slice(128 * m, 128 * (m + 1))
        sp = ps2b.tile([128, 4], dt.float32, name=f"s2bp{m}", space="PSUM")
        for k in range(4):
            kr = (128, 128, 128, 46)[k]
            nc.tensor.matmul(sp[:], w2bT_r[k][:, msl], sh2r[k][0:kr, :],
                             start=(k == 0), stop=(k == 3))
        st = per.tile([128, 1], dt.float32, name=f"s2b_{m}")
        nc.scalar.copy(st[:], sp[:, 0:1])
        s2b.append(st)
    ps2b.release()
    q2b = [_acc_reduce(nc, per, sq2b[m][:], 128, f"q2br{m}") for m in range(4)]
    ar4 = _allreduce(nc, dram, per, [s[:] for s in s2b], [q[:] for q in q2b],
                     rows2b, 512, "ar4")
    sb2b = [_bn_coeffs(nc, per, ar4[m][0][:], ar4[m][1][:], g2b[m][:], b2b[m][:],
                       128, CNT, f"bn2b{m}") for m in range(4)]

    # x2 = prelu(bn(m2)) -> bf16; x1 recast to bf16 for block 3
    c3t = tc.alloc_tile_pool(name="c3t", bufs=4)
    px2r = tc.alloc_tile_pool(name="px2r", bufs=1)
    x2r = [px2r.tile([128, N], dt.bfloat16, name=f"x2r_{i}") for i in range(4)]
    for m in range(4):
        nc.scalar.activation(x2r[m][:], m2[m][:], AF.Prelu, bias=sb2b[m][1][:],
                             scale=sb2b[m][0][:], alpha=0.2)
    xb1 = [px2r.tile([r, N], dt.bfloat16, name=f"xb1_{i}")
           for i, r in enumerate((128, 53))]
    for i in range(2):
        nc.vector.tensor_copy(xb1[i][:], x1hi[i][:].bitcast(dt.float32))

    # ---------------- block 3 ----------------
    # y3a / y3b stay SBUF-resident in bf16 (post-x1 path tolerates bf16)
    xc_k = [xb1[0], xb1[1], x2r[0], x2r[1], x2r[2], x2r[3]]
    rows3a = (128, 128, 128, 128, 83)
    y3a_s = [pb3.tile([r, N], dt.bfloat16, name=f"y3a_s{i}") for i, r in enumerate(rows3a)]
    sum3a = [per.tile([r, 4], dt.float32, name=f"s3a_{i}") for i, r in enumerate(rows3a)]
    sq3a = [per.tile([r, 4], dt.float32, name=f"q3a_{i}") for i, r in enumerate(rows3a)]
    c3p = tc.alloc_tile_pool(name="c3p", bufs=4, space="PSUM")
    for c in range(4):
        csl = slice(512 * c, 512 * (c + 1))
        for m, rows in enumerate(rows3a):
            msl = slice(128 * m, 128 * m + rows)
            yp = c3p.tile([128, 512], dt.float32, name="y3ap", space="PSUM")
            for k in range(6):
                nc.tensor.matmul(yp[:rows, :], w3aT_r[k][:, msl], xc_k[k][:, csl],
                                 start=(k == 0), stop=(k == 5))
            nc.scalar.activation(y3a_s[m][:, csl], yp[:rows, :], AF.Copy,
                                 accum_out=sum3a[m][:, c:c + 1])
            tr = c3t.tile([rows, 512], dt.bfloat16, name="tr")
            nc.scalar.activation(tr[:], yp[:rows, :], AF.Square,
                                 accum_out=sq3a[m][:, c:c + 1])
    c3p.release()
    px2r.release()

    s3 = [_acc_reduce(nc, per, sum3a[m][:], rows3a[m], f"s3r{m}") for m in range(5)]
    q3 = [_acc_reduce(nc, per, sq3a[m][:], rows3a[m], f"q3r{m}") for m in range(5)]
    ar5 = _allreduce(nc, dram, per, [s[:] for s in s3], [q[:] for q in q3],
                     rows3a, 640, "ar5")
    sb3a = [_bn_coeffs(nc, per, ar5[m][0][:], ar5[m][1][:], g3a[m][:], b3a[m][:],
                       rows3a[m], CNT3, f"bn3a{m}") for m in range(5)]

    rows3b = (128, 128, 128, 128)
    y3b_s = [pb3.tile([r, N], dt.bfloat16, name=f"y3b_s{i}") for i, r in enumerate(rows3b)]
    sum3b = [per.tile([r, 4], dt.float32, name=f"s3b_{i}") for i, r in enumerate(rows3b)]
    sq3b = [per.tile([r, 4], dt.float32, name=f"q3b_{i}") for i, r in enumerate(rows3b)]
    c4p = tc.alloc_tile_pool(name="c4p", bufs=2, space="PSUM")
    h3l = tc.alloc_tile_pool(name="h3l", bufs=10)
    for c in range(4):
        csl = slice(512 * c, 512 * (c + 1))
        h3c = []
        for k, kr in enumerate(rows3a):
            ld = h3l.tile([kr, 512], dt.bfloat16, name="h3ld")
            nc.scalar.activation(ld[:], y3a_s[k][:, csl], AF.Prelu,
                                 bias=sb3a[k][1][:], scale=sb3a[k][0][:], alpha=0.2)
            h3c.append(ld)
        for m, rows in enumerate(rows3b):
            msl = slice(128 * m, 128 * (m + 1))
            yp = c4p.tile([rows, 512], dt.float32, name=f"y3bp{m}", space="PSUM")
            for k in range(5):
                nc.tensor.matmul(yp[:], w3bT_r[k][:, msl], h3c[k][:],
                                 start=(k == 0), stop=(k == 4))
            nc.scalar.activation(y3b_s[m][:, csl], yp[:], AF.Copy,
                                 accum_out=sum3b[m][:, c:c + 1])
            tr = c3t.tile([rows, 512], dt.bfloat16, name="tr")
            nc.scalar.activation(tr[:], yp[:], AF.Square,
                                 accum_out=sq3b[m][:, c:c + 1])
    c4p.release()
    h3l.release()
    c3t.release()

    s4 = [_acc_reduce(nc, per, sum3b[m][:], 128, f"s4r{m}") for m in range(4)]
    q4 = [_acc_reduce(nc, per, sq3b[m][:], 128, f"q4r{m}") for m in range(4)]
    ar6 = _allreduce(nc, dram, per, [s[:] for s in s4], [q[:] for q in q4],
                     rows3b, 512, "ar6")
    sb3b = [_bn_coeffs(nc, per, ar6[m][0][:], ar6[m][1][:], g3b[m][:], b3b[m][:],
                       128, CNT3, f"bn3b{m}") for m in range(4)]

    out_d = nc._ext_outputs["out"]
    for m in range(4):
        for c in range(4):
            csl = slice(512 * c, 512 * (c + 1))
            ot = oseg.tile([128, 512], dt.float32, name="ot")
            nc.scalar.activation(ot[:], y3b_s[m][:, csl], AF.Prelu,
                                 bias=sb3b[m][1][:], scale=sb3b[m][0][:], alpha=0.2)
            nc.sync.dma_start(out_d[128 * m:128 * (m + 1), csl], ot[:])
    pb3.release()
    pw3.release()
    b2bpool.release()
    b2s.release()
    b2g.release()
    wld.release()
    oseg.release()
    b1t.release()
    per.release()


# ------------------------------------------------------------------ host side
_CACHE = {}


def _build():
    _install_bassfix()
    nc = bass.Bass("TRN2", target_bir_lowering=False, debug=False,
                   num_devices=NCORES)
    in_specs = {
        "x": (64, N), "w1aT": (64, 152), "du1T": (64, 152),
        "w1bT_k0": (128, 181), "w1bT_k1": (24, 181),
        "w2aT_k0": (128, 430), "w2aT_k1": (53, 430),
        "du2T_k0": (128, 430), "du2T_k1": (53, 430),
        "w2bT_k0": (128, 512), "w2bT_k1": (128, 512),
        "w2bT_k2": (128, 512), "w2bT_k3": (46, 512),
        "w3aT_k0": (128, 595), "w3aT_k1": (53, 595), "w3aT_k2": (128, 595),
        "w3aT_k3": (128, 595), "w3aT_k4": (128, 595), "w3aT_k5": (128, 595),
        "w3bT_k0": (128, 512), "w3bT_k1": (128, 512), "w3bT_k2": (128, 512),
        "w3bT_k3": (128, 512), "w3bT_k4": (83, 512),
        "g1a": (152, 1), "be1a": (152, 1), "g1b": (181, 1), "be1b": (181, 1),
        "g2a": (430, 1), "be2a": (430, 1), "g2b": (512, 1), "be2b": (512, 1),
        "g3a": (595, 1), "be3a": (595, 1), "g3b": (512, 1), "be3b": (512, 1),
        "E": (32, CB),
    }
    nc._ext_inputs = {}
    for nm, shp in in_specs.items():
        nc._ext_inputs[nm] = nc.dram_tensor(nm, list(shp), dt.float32,
                                            kind="ExternalInput").ap()
    nc._ext_outputs = {
        "out": nc.dram_tensor("out", [512, N], dt.float32,
                              kind="ExternalOutput").ap()}
    with tile.TileContext(nc) as tc:
        build_kernel(nc, tc)
    return nc


def _host_inputs(x, c1w1, c1g1, c1be1, c1w2, c1g2, c1be2,
                 c2w1, c2g1, c2be1, c2w2, c2g2, c2be2,
                 c3w1, c3g1, c3be1, c3w2, c3g2, c3be2):
    f32 = np.float32
    W1 = np.asarray(c1w1, f32)            # [152, 128]
    W1a, W1b = W1[:, :64], W1[:, 64:]
    W2 = np.asarray(c2w1, f32)            # [430, 362]
    W2a, W2b = W2[:, :181], W2[:, 181:]
    ws = {
        "w1aT": np.ascontiguousarray(W1a.T),
        "du1T": np.ascontiguousarray((W1b - W1a).T),
    }
    w1bT = np.ascontiguousarray(np.asarray(c1w2, f32).T)     # [152, 181]
    ws["w1bT_k0"], ws["w1bT_k1"] = w1bT[:128], w1bT[128:]
    w2aT = np.ascontiguousarray(W2a.T)                        # [181, 430]
    du2T = np.ascontiguousarray((W2b - W2a).T)
    ws["w2aT_k0"], ws["w2aT_k1"] = w2aT[:128], w2aT[128:]
    ws["du2T_k0"], ws["du2T_k1"] = du2T[:128], du2T[128:]
    w2bT = np.ascontiguousarray(np.asarray(c2w2, f32).T)     # [430, 512]
    for i, sl in enumerate((slice(0, 128), slice(128, 256), slice(256, 384),
                            slice(384, 430))):
        ws[f"w2bT_k{i}"] = w2bT[sl]
    w3aT = np.ascontiguousarray(np.asarray(c3w1, f32).T)     # [693, 595]
    cuts = (0, 128, 181, 309, 437, 565, 693)
    for i in range(6):
        ws[f"w3aT_k{i}"] = w3aT[cuts[i]:cuts[i + 1]]
    w3bT = np.ascontiguousarray(np.asarray(c3w2, f32).T)     # [595, 512]
    for i in range(5):
        ws[f"w3bT_k{i}"] = w3bT[128 * i:min(128 * (i + 1), 595)]
    for nm, v in (("g1a", c1g1), ("be1a", c1be1), ("g1b", c1g2), ("be1b", c1be2),
                  ("g2a", c2g1), ("be2a", c2be1), ("g2b", c2g2), ("be2b", c2be2),
                  ("g3a", c3g1), ("be3a", c3be1), ("g3b", c3g2), ("be3b", c3be2)):
        ws[nm] = np.ascontiguousarray(np.asarray(v, f32).reshape(-1, 1))
    E = np.zeros((32, CB), f32)
    for r in range(32):
        E[r, KNN * r:KNN * (r + 1)] = 1.0
    ws["E"] = E
    ws = {k: np.ascontiguousarray(v, f32) for k, v in ws.items()}
    xs = np.asarray(x, f32)
    in_maps = []
    for i in range(NCORES):
        m = dict(ws)
        m["x"] = np.ascontiguousarray(xs[i])
        in_maps.append(m)
    return in_maps


def kernel(x, c1w1, c1b1, c1g1, c1be1, c1w2, c1b2, c1g2, c1be2,
           c2w1, c2b1, c2g1, c2be1, c2w2, c2b2, c2g2, c2be2,
           c3w1, c3b1, c3g1, c3be1, c3w2, c3b2, c3g2, c3be2):
    # conv biases are absorbed exactly by the following BatchNorm (shift
    # invariance), so b* inputs are unused by the device program.
    if "nc" not in _CACHE:
        _CACHE["nc"] = _build()
    nc = _CACHE["nc"]
    in_maps = _host_inputs(x, c1w1, c1g1, c1be1, c1w2, c1g2, c1be2,
                           c2w1, c2g1, c2be1, c2w2, c2g2, c2be2,
                           c3w1, c3g1, c3be1, c3w2, c3g2, c3be2)
    res = run_bass_kernel_spmd(nc, in_maps, core_ids=list(range(NCORES)))
    _CACHE["last_result"] = res
    out = np.stack([res.results[i]["out"] for i in range(NCORES)], axis=0)
    return out.astype(np.float32)



# revision 39
# speedup vs baseline: 1.1580x; 1.0437x over previous
"""DGCNN-style GCN encoder on 8 TRN2 NeuronCores (bass/tile).

Data-parallel over batch: each core owns one sample (B=8). BatchNorm batch
statistics are all-reduced across the 8 cores so results match global-batch
BN exactly. kNN top-10, neighbor gathers, and all convs/BN/activations run
on-device; the host only reshapes/replicates weights.

Precision strategy: matmuls on the x -> x1 -> knn2 path use a compensated
fp32r (hi/lo split) scheme that recovers ~fp32 accuracy at fp32r speed
(conv1b uses plain fp32). This keeps the kNN neighbor selection stable
against the f32 reference. Everything downstream of x1 (conv2a/2b, block 3)
runs in bf16 (weights + activations), which the 2e-2 tolerance absorbs.

Performance notes (vs the first working version):
- edge loops process 1280-edge chunks (128 points x 10 nbrs) instead of 320
- neighbor gathers are indirect_copy, whose cost is ~28ns per index of Q7
  dispatch; conv2a gathers both x1 row-tiles in ONE pass by interleaving
  them as bf16 pairs (d=2 gather with doubled indices), halving index count
- gathers/loads are software-pipelined one-to-two chunks ahead, and the
  next phase's first loads + weight loads are issued before each BN
  AllReduce so DMA overlaps the collective
- block-1's gather stream overlaps knn1 (per-tile iw builds + pools
  pre-allocated before the knn pass so gather t only waits on top-k tile t);
  iw2/iw2d are likewise built per-tile during knn2
- knn2's lo-correction matmuls (4 of 6) run in bf16 at 1 cyc/col; only the
  two hi*hi + sq-fold matmuls stay compensated f32r, keeping neighbor
  selection error ~5e-5 absolute
- y3a/y3b stay SBUF-resident in bf16 (no DRAM round-trip); y2a is spilled
  to DRAM in bf16 (half the traffic)
"""

import os
import sys
import json

B16 = "/nix/store/wxap7svlj45h0lfm31d1axjjnzyl6qsy-b16-bazel-unstable-cc-2026-05-04-9a3fa1f3-rt-2026-05-04-ade39e0a/lib/python3.13/site-packages"
if B16 not in sys.path:
    sys.path.insert(0, B16)
if "/opt/trn_rl_repo" not in sys.path:
    sys.path.insert(0, "/opt/trn_rl_repo")

import numpy as np
import concourse.bass as bass
import concourse.mybir as mybir
import concourse.tile as tile
from concourse.bass_utils import run_bass_kernel_spmd

dt = mybir.dt
AF = mybir.ActivationFunctionType
ALU = mybir.AluOpType
AX = mybir.AxisListType

N = 2048
KNN = 10
S = N * KNN            # 20480 edge positions
CB = 1280              # banded chunk: 128 points x 10 neighbors
NCHUNK = S // CB       # 16
CSL = [(0, 512), (512, 1024), (1024, 1280)]   # matmul col slices of a chunk
EPS = 1e-5
NCORES = 8
CNT = float(NCORES * S)      # BN count for edge tensors
CNT3 = float(NCORES * N)     # BN count for block-3 tensors


# ---------------------------------------------------------------- walrus fix
def _fix_bir_json(bir_json: bytes) -> bytes:
    """This container's walrus accepts only ONE sync-wait per instruction;
    hoist extra waits onto preceding EventSemaphore instructions."""
    m = json.loads(bir_json)
    for f in m["functions"]:
        for bb in f["blocks"]:
            out = []
            for i in bb["instructions"]:
                si = i.get("sync_info") or {}
                waits = si.get("on_wait") or []
                if len(waits) > 1:
                    for k, w in enumerate(waits[:-1]):
                        out.append({
                            "name": f"{i['name']}w{k}",
                            "engine": i["engine"],
                            "opcode": "EventSemaphore",
                            "ins": [], "outs": [],
                            "debug": i.get("debug"),
                            "sync_info": {"on_update": [], "on_wait": [w]},
                        })
                    i = dict(i)
                    si = dict(si)
                    si["on_wait"] = [waits[-1]]
                    i["sync_info"] = si
                out.append(i)
            bb["instructions"] = out
    return json.dumps(m).encode()


def _install_bassfix():
    import concourse.bass_utils as bu
    import concourse.bass2jax as b2j
    if getattr(bu, "_bassfix_installed", False):
        return
    real = bu.compile_bir_kernel

    def patched(bir_json, tmpdir, neff_name="file.neff"):
        return real(_fix_bir_json(bir_json), tmpdir, neff_name)

    bu.compile_bir_kernel = patched
    b2j.compile_bir_kernel = patched
    bu._bassfix_installed = True


# ------------------------------------------------------------- device helpers
def _round_split(nc, pool, src_ap, parts, width, name):
    """Return (hi, lo) float32r tiles for a f32 source AP [parts, width]."""
    hi = pool.tile([parts, width], dt.float32r, name=name + "_hi")
    lo = pool.tile([parts, width], dt.float32r, name=name + "_lo")
    nc.vector.tensor_copy(hi[:], src_ap)
    nc.vector.tensor_tensor(out=lo[:], in0=src_ap, in1=hi[:].bitcast(dt.float32),
                            op=ALU.subtract)
    return hi, lo


def _bn_coeffs(nc, pool, sums, sqs, g_ap, b_ap, rows, cnt, name):
    """sums/sqs: [rows,1] f32 (global). Returns (scale, bias) [rows,1] f32."""
    mean = pool.tile([rows, 1], dt.float32, name=name + "_mean")
    nc.vector.tensor_scalar_mul(mean[:], sums, 1.0 / cnt)
    e2 = pool.tile([rows, 1], dt.float32, name=name + "_e2")
    nc.vector.tensor_scalar_mul(e2[:], sqs, 1.0 / cnt)
    m2 = pool.tile([rows, 1], dt.float32, name=name + "_m2")
    nc.vector.tensor_tensor(out=m2[:], in0=mean[:], in1=mean[:], op=ALU.mult)
    var = pool.tile([rows, 1], dt.float32, name=name + "_var")
    nc.vector.tensor_tensor(out=var[:], in0=e2[:], in1=m2[:], op=ALU.subtract)
    epst = pool.tile([rows, 1], dt.float32, name=name + "_eps")
    nc.vector.memset(epst[:], float(EPS))
    std = pool.tile([rows, 1], dt.float32, name=name + "_std")
    nc.scalar.activation(std[:], var[:], AF.Sqrt, bias=epst[:])
    rstd = pool.tile([rows, 1], dt.float32, name=name + "_rstd")
    nc.vector.reciprocal(rstd[:], std[:])
    scale = pool.tile([rows, 1], dt.float32, name=name + "_scale")
    nc.vector.tensor_tensor(out=scale[:], in0=rstd[:], in1=g_ap, op=ALU.mult)
    ms = pool.tile([rows, 1], dt.float32, name=name + "_ms")
    nc.vector.tensor_tensor(out=ms[:], in0=mean[:], in1=scale[:], op=ALU.mult)
    bias = pool.tile([rows, 1], dt.float32, name=name + "_bias")
    nc.vector.tensor_tensor(out=bias[:], in0=b_ap, in1=ms[:], op=ALU.subtract)
    return scale, bias


def _allreduce(nc, dram, per, sums_list, sqs_list, rows_list, cpad, name):
    """Pack per-Mtile [rows,1] sum/sq into a DRAM [cpad,2] buffer, AllReduce
    across the 8 cores, unpack back into fresh [rows,1] tiles."""
    cin = dram.tile([cpad, 2], dt.float32, name=name + "_in")
    cout = dram.tile([cpad, 2], dt.float32, name=name + "_out")
    r0 = 0
    for s_, q_, rows in zip(sums_list, sqs_list, rows_list):
        nc.sync.dma_start(cin[r0:r0 + rows, 0:1], s_)
        nc.sync.dma_start(cin[r0:r0 + rows, 1:2], q_)
        r0 += rows
    if r0 < cpad:
        z = per.tile([cpad - r0, 2], dt.float32, name=name + "_z")
        nc.vector.memset(z[:], 0.0)
        nc.sync.dma_start(cin[r0:cpad, :], z[:])
    nc.gpsimd.collective_compute(
        "AllReduce", ALU.add, replica_groups=[list(range(NCORES))],
        ins=[cin[:].opt()], outs=[cout[:].opt()])
    outs = []
    r0 = 0
    for rows in rows_list:
        gs = per.tile([rows, 1], dt.float32, name=name + f"_gs{r0}")
        gq = per.tile([rows, 1], dt.float32, name=name + f"_gq{r0}")
        nc.sync.dma_start(gs[:], cout[r0:r0 + rows, 0:1])
        nc.sync.dma_start(gq[:], cout[r0:r0 + rows, 1:2])
        outs.append((gs, gq))
        r0 += rows
    return outs


def _acc_reduce(nc, per, acc, rows, name):
    """[rows, NCHUNK] accumulator -> [rows,1]."""
    out = per.tile([rows, 1], dt.float32, name=name)
    nc.vector.tensor_reduce(out=out[:], in_=acc, axis=AX.X, op=ALU.add)
    return out


def build_kernel(nc, tc):
    P = 128
    per = tc.alloc_tile_pool(name="persist", bufs=1)
    dram = tc.alloc_tile_pool(name="dram", bufs=1, space="DRAM")
    b1t = tc.alloc_tile_pool(name="b1t", bufs=2)
    oseg = tc.alloc_tile_pool(name="oseg", bufs=1)
    wld = tc.alloc_tile_pool(name="wld", bufs=2)
    ph1 = tc.alloc_tile_pool(name="ph1", bufs=1)

    ins = nc._ext_inputs
    x_d = ins["x"]

    # ---------------- load & prep ----------------
    tmp0 = tc.alloc_tile_pool(name="tmp0", bufs=1)      # until U1t/rhs built
    x = tmp0.tile([64, N], dt.float32, name="x")
    nc.sync.dma_start(x[:], x_d)
    xhi, xlo = _round_split(nc, tmp0, x[:], 64, N, "x")
    # stacks for compensated matmuls
    xstack = ph1.tile([P, N], dt.float32r, name="xstack")   # [xhi; xlo]
    nc.vector.tensor_copy(xstack[:64, :], xhi[:])
    nc.vector.tensor_copy(xstack[64:, :], xlo[:])
    lhsx = xstack                                             # [xhi; xlo]
    rhsx = ph1.tile([P, N], dt.float32r, name="rhsx")        # 2*[xlo; xhi]
    nc.vector.tensor_scalar_mul(rhsx[:64, :], xlo[:], 2.0)
    nc.vector.tensor_scalar_mul(rhsx[64:, :], xhi[:], 2.0)

    # sq1 = sum_c x^2 (compensated)
    xsq = tmp0.tile([64, N], dt.float32, name="xsq")
    nc.vector.tensor_tensor(out=xsq[:], in0=x[:], in1=x[:], op=ALU.mult)
    xsqhi, xsqlo = _round_split(nc, tmp0, xsq[:], 64, N, "xsq")
    ones_f = per.tile([128, 1], dt.float32, name="ones_f")
    nc.vector.memset(ones_f[:], 1.0)
    ones64 = per.tile([64, 1], dt.float32r, name="ones64")
    nc.vector.tensor_copy(ones64[:], ones_f[:64, :])
    # knn1 matmul operands (sq1 folded in chunk-wise)
    lhs_hi1 = ph1.tile([66, N], dt.float32r, name="lhs_hi1")
    nc.vector.tensor_copy(lhs_hi1[:64, :], xhi[:])
    ones2f = tmp0.tile([2, N], dt.float32, name="ones2f")
    nc.vector.memset(ones2f[:], 1.0)
    nc.vector.tensor_copy(lhs_hi1[64:66, :], ones2f[:])
    rhs_hi1 = ph1.tile([66, N], dt.float32r, name="rhs_hi1")
    nc.vector.tensor_scalar_mul(rhs_hi1[:64, :], xhi[:], 2.0)
    psq = tc.alloc_tile_pool(name="psq", bufs=1, space="PSUM")
    sq1p = psq.tile([1, N], dt.float32, name="sq1p", space="PSUM")
    for c in range(4):
        sl = slice(512 * c, 512 * (c + 1))
        nc.tensor.matmul(sq1p[:, sl], ones64[:], xsqhi[:, sl], start=True, stop=False)
        nc.tensor.matmul(sq1p[:, sl], ones64[:], xsqlo[:, sl], start=False, stop=True)
        sc_ = tmp0.tile([1, 512], dt.float32, name="sq1c")
        nc.scalar.copy(sc_[:], sq1p[:, sl])
        sh_ = tmp0.tile([1, 512], dt.float32r, name="sq1ch")
        nc.vector.tensor_copy(sh_[:], sc_[:])
        sl_ = tmp0.tile([1, 512], dt.float32r, name="sq1cl")
        nc.vector.tensor_tensor(out=sl_[:], in0=sc_[:], in1=sh_[:].bitcast(dt.float32),
                                op=ALU.subtract)
        nc.vector.tensor_scalar_mul(rhs_hi1[64:65, sl], sh_[:], -1.0)
        nl_ = tmp0.tile([1, 512], dt.float32r, name="sq1nl")
        nc.vector.tensor_scalar_mul(nl_[:], sl_[:], -1.0)
        nc.sync.dma_start(rhs_hi1[65:66, sl], nl_[:])
    psq.release()

    # conv1 weights
    w1aT = wld.tile([128, 595], dt.float32, name="wf")
    nc.sync.dma_start(w1aT[:64, :152], ins["w1aT"])
    w1aT_hi, w1aT_lo = _round_split(nc, per, w1aT[:64, :152], 64, 152, "w1aT")
    du1T = wld.tile([128, 595], dt.float32, name="wf")
    nc.sync.dma_start(du1T[:64, :152], ins["du1T"])
    du1T_hi, du1T_lo = _round_split(nc, per, du1T[:64, :152], 64, 152, "du1T")
    w1aX = per.tile([P, 152], dt.float32r, name="w1aX")      # [Wlo; Whi]
    nc.vector.tensor_copy(w1aX[:64, :], w1aT_lo[:])
    nc.vector.tensor_copy(w1aX[64:, :], w1aT_hi[:])
    du1X = per.tile([P, 152], dt.float32r, name="du1X")      # [dlo; dhi] pairs lhsx
    nc.vector.tensor_copy(du1X[:64, :], du1T_lo[:])
    nc.vector.tensor_copy(du1X[64:, :], du1T_hi[:])

    wkt1b = []  # conv1b fp32 weights, K-tiles
    for i, rows in enumerate((128, 24)):
        t = per.tile([rows, 181], dt.float32, name=f"w1bT_k{i}")
        nc.sync.dma_start(t[:], ins[f"w1bT_k{i}"])
        wkt1b.append(t)


    # U1[c, n] = sum_ci du1[ci, c] x[ci, n]   (compensated, natural layout)
    rows1a_ = (128, 24)
    u1 = [ph1.tile([r, N], dt.float32, name=f"u1_{i}") for i, r in enumerate(rows1a_)]
    pu = tc.alloc_tile_pool(name="pu", bufs=4, space="PSUM")
    for m, rows in enumerate(rows1a_):
        msl = slice(128 * m, 128 * m + rows)
        for cc in range(4):
            sl = slice(512 * cc, 512 * (cc + 1))
            up = pu.tile([rows, 512], dt.float32, name="up", space="PSUM")
            nc.tensor.matmul(up[:], du1T_hi[:, msl], xhi[:, sl], start=True, stop=False)
            nc.tensor.matmul(up[:], du1X[:, msl], xstack[:, sl], start=False, stop=True)
            nc.scalar.copy(u1[m][:, sl], up[:])
    pu.release()
    tmp0.release()

    # BN params
    def load_vec(nm, rows_list):
        tiles = []
        r0 = 0
        for i, rows in enumerate(rows_list):
            t = per.tile([rows, 1], dt.float32, name=f"{nm}_{i}")
            nc.sync.dma_start(t[:], ins[nm][r0:r0 + rows, :])
            tiles.append(t)
            r0 += rows
        return tiles

    g1a = load_vec("g1a", (128, 24)); b1a = load_vec("be1a", (128, 24))
    g1b = load_vec("g1b", (128, 53)); b1b = load_vec("be1b", (128, 53))
    g2a = load_vec("g2a", (128, 128, 128, 46)); b2a = load_vec("be2a", (128, 128, 128, 46))
    g2b = load_vec("g2b", (128,) * 4); b2b = load_vec("be2b", (128,) * 4)
    g3a = load_vec("g3a", (128,) * 4 + (83,)); b3a = load_vec("be3a", (128,) * 4 + (83,))
    g3b = load_vec("g3b", (128,) * 4); b3b = load_vec("be3b", (128,) * 4)

    # ---------------- kNN (shared machinery) ----------------
    idx1_d = dram.tile([16, P, KNN], dt.uint16, name="idx1_d")
    idx2_d = dram.tile([16, P, KNN], dt.uint16, name="idx2_d")

    def knn_pass(mm_emit, idx_dram, tag, after_tile=None, kp_bufs=2):
        """mm_emit(psum_slice, c) emits matmuls for columns 512c..512c+512."""
        kp = tc.alloc_tile_pool(name=f"knnp_{tag}", bufs=kp_bufs, space="PSUM")
        ks = tc.alloc_tile_pool(name=f"knns_{tag}", bufs=2)
        for t in range(16):
            pe = kp.tile([P, N], dt.float32, name="pe", space="PSUM")
            for c in range(4):
                mm_emit(pe, t, c)
            q = ks.tile([P, N], dt.float32, name="q")
            nc.scalar.copy(q[:], pe[:])
            v8 = ks.tile([P, 8], dt.float32, name="v8")
            i8 = ks.tile([P, 8], dt.uint16, name="i8")
            nc.vector.max(out=v8[:], in_=q[:])
            nc.vector.max_index(out=i8[:], in_max=v8[:], in_values=q[:])
            nc.vector.match_replace(out=q[:], in_to_replace=v8[:], in_values=q[:],
                                    imm_value=-1e30)
            v8b = ks.tile([P, 8], dt.float32, name="v8b")
            i8b = ks.tile([P, 8], dt.uint16, name="i8b")
            nc.vector.max(out=v8b[:], in_=q[:])
            nc.vector.max_index(out=i8b[:], in_max=v8b[:], in_values=q[:])
            idx = ks.tile([P, KNN], dt.uint16, name="idx")
            nc.vector.tensor_copy(idx[:, 0:7], i8[:, 1:8])
            nc.vector.tensor_copy(idx[:, 7:10], i8b[:, 0:3])
            nc.sync.dma_start(idx_dram[t], idx[:])
            if after_tile is not None:
                after_tile(t)
        kp.release()
        ks.release()

    def build_wrapped(idx_dram, tag):
        """DRAM flat idx (point-major) -> wrapped [128, S/16] uint16."""
        iw = per.tile([P, S // 16], dt.uint16, name=f"iw_{tag}")
        flat = idx_dram[:].rearrange("a p k -> (a p k)")
        nc.sync.dma_start(iw[0:16, :], flat.rearrange("(f p) -> p f", p=16))
        for g in range(1, 8):
            nc.sync.dma_start(iw[16 * g:16 * (g + 1), :], iw[0:16, :])
        return iw

    # knn1 matmuls
    def mm1(pe, t, c):
        msl = slice(128 * t, 128 * (t + 1))
        csl = slice(512 * c, 512 * (c + 1))
        nc.tensor.matmul(pe[:, csl], lhs_hi1[:, msl], rhs_hi1[:, csl],
                         start=True, stop=False)
        nc.tensor.matmul(pe[:, csl], lhsx[:, msl], rhsx[:, csl],
                         start=False, stop=True)

    # ---------------- block 1, interleaved with knn1 ----------------
    # y1a[(n,j)] = W1a @ (nb - cen) + W1b @ cen ; stats; keep for BN.
    # Each knn1 tile's top-k feeds its 4 stats chunks immediately, so the
    # gather-paced loop overlaps the DVE top-k of later tiles.
    y1a_d = [dram.tile([P, S], dt.float32, name="y1a_d0"),
             dram.tile([24, S], dt.float32, name="y1a_d1")]
    rows1a = (128, 24)
    sum1a = [ph1.tile([r, NCHUNK], dt.float32, name=f"s1a_{i}") for i, r in enumerate(rows1a)]
    sq1a = [ph1.tile([r, NCHUNK], dt.float32, name=f"q1a_{i}") for i, r in enumerate(rows1a)]
    # b1 pools pre-allocated so the gather stream overlaps knn1; iw1 built
    # per-tile so gather t only depends on top-k tile t
    iw1 = per.tile([P, S // 16], dt.uint16, name="iw_k1")
    b1p = tc.alloc_tile_pool(name="b1p", bufs=1, space="PSUM")
    b1s = tc.alloc_tile_pool(name="b1s", bufs=3)
    knn_pass(mm1, idx1_d, "k1", kp_bufs=1)

    b1q = {}
    def b1_gather(t):
        cols = slice(80 * t, 80 * (t + 1))
        flat_t = idx1_d[t].rearrange("p k -> (p k)")
        nc.sync.dma_start(iw1[0:16, cols], flat_t.rearrange("(f p) -> p f", p=16))
        nc.sync.dma_start(iw1[16:32, cols], iw1[0:16, cols])
        nc.sync.dma_start(iw1[32:64, cols], iw1[0:32, cols])
        nc.sync.dma_start(iw1[64:128, cols], iw1[0:64, cols])
        nbr = b1s.tile([P, CB], dt.float32, name="nbr")
        for g in range(2):
            nc.gpsimd.indirect_copy(out=nbr[:, 640 * g:640 * (g + 1)],
                                    data=xstack[:].bitcast(dt.float32),
                                    idxs=iw1[:, 80 * t + 40 * g:80 * t + 40 * (g + 1)],
                                    i_know_ap_gather_is_preferred=True)
        nbxt = b1s.tile([P, CB], dt.float32r, name="nbxt")
        nc.scalar.copy(nbxt[:], nbr[:])
        b1q[t] = nbxt

    b1_gather(0)
    for t in range(NCHUNK):
        if t + 1 < NCHUNK:
            b1_gather(t + 1)
        nbx = b1q.pop(t)[:]
        csl = slice(CB * t, CB * (t + 1))
        for m, rows in enumerate(rows1a):
            msl = slice(128 * m, 128 * m + rows)
            yp = b1p.tile([P, CB], dt.float32, name="yp", space="PSUM")
            for c0, c1 in CSL:
                nc.tensor.matmul(yp[:rows, c0:c1], w1aT_hi[:, msl],
                                 nbx[0:64, c0:c1], start=True, stop=False)
                nc.tensor.matmul(yp[:rows, c0:c1], w1aX[:, msl],
                                 nbx[:, c0:c1], start=False, stop=True)
            uview = u1[m][:, 128 * t:128 * (t + 1), None].to_broadcast(
                [rows, 128, KNN])
            ob = b1s.tile([rows, CB], dt.float32, name="ob1")
            nc.vector.scalar_tensor_tensor(
                out=ob[:].rearrange("p (n k) -> p n k", k=KNN),
                in0=yp[:rows, :].rearrange("p (n k) -> p n k", k=KNN), scalar=1.0,
                in1=uview, op0=ALU.mult, op1=ALU.add,
                accum_out=sum1a[m][:, t:t + 1])
            nc.sync.dma_start(y1a_d[m][:, csl], ob[:])
            tr = b1t.tile([rows, CB], dt.bfloat16, name="tr")
            nc.scalar.activation(tr[:], ob[:], AF.Square,
                                 accum_out=sq1a[m][:, t:t + 1])
    b1s.release()
    b1p.release()
    k2pool = tc.alloc_tile_pool(name="k2pool", bufs=1)

    s1 = [_acc_reduce(nc, per, sum1a[m][:], rows1a[m], f"s1r{m}") for m in range(2)]
    q1 = [_acc_reduce(nc, per, sq1a[m][:], rows1a[m], f"q1r{m}") for m in range(2)]

    # h1 load prefetch issued before the AllReduce so DMA overlaps it
    sumh1 = [ph1.tile([r, NCHUNK], dt.float32, name=f"sh1_{i}") for i, r in enumerate(rows1a)]
    rows1b = (128, 53)
    m1 = [k2pool.tile([r, N], dt.float32, name=f"m1_{i}") for i, r in enumerate(rows1b)]
    sq1b = [ph1.tile([r, NCHUNK], dt.float32, name=f"q1b_{i}") for i, r in enumerate(rows1b)]
    b2pp = tc.alloc_tile_pool(name="b2pp", bufs=2, space="PSUM")
    h1l = tc.alloc_tile_pool(name="h1l", bufs=2)

    h1q = {}
    def h1_load(c):
        csl = slice(CB * c, CB * (c + 1))
        pair = []
        for m, rows in enumerate(rows1a):
            ld = h1l.tile([rows, CB], dt.float32, name=f"h1ld{m}")
            nc.sync.dma_start(ld[:], y1a_d[m][:, csl])
            pair.append(ld)
        h1q[c] = pair

    h1_load(0)
    ar1 = _allreduce(nc, dram, per, [s1[0][:], s1[1][:]], [q1[0][:], q1[1][:]],
                     rows1a, 152, "ar1")
    sb1a = [_bn_coeffs(nc, per, ar1[m][0][:], ar1[m][1][:], g1a[m][:], b1a[m][:],
                       rows1a[m], CNT, f"bn1a{m}") for m in range(2)]
    for c in range(NCHUNK):
        if c + 1 < NCHUNK:
            h1_load(c + 1)
        csl = slice(CB * c, CB * (c + 1))
        h1c = h1q.pop(c)
        for m, rows in enumerate(rows1a):
            ld = h1c[m]
            nc.scalar.activation(ld[:], ld[:], AF.Prelu,
                                 bias=sb1a[m][1][:], scale=sb1a[m][0][:], alpha=0.2,
                                 accum_out=sumh1[m][:, c:c + 1])
        for m, rows in enumerate(rows1b):
            msl = slice(128 * m, 128 * m + rows)
            yp = b2pp.tile([P, CB], dt.float32, name="y1bp", space="PSUM")
            for c0, c1 in CSL:
                nc.tensor.matmul(yp[:rows, c0:c1], wkt1b[0][:, msl],
                                 h1c[0][:, c0:c1], start=True, stop=False)
                nc.tensor.matmul(yp[:rows, c0:c1], wkt1b[1][:, msl],
                                 h1c[1][:, c0:c1], start=False, stop=True)
            tr = b1t.tile([rows, CB], dt.bfloat16, name="tr")
            nc.scalar.activation(tr[:], yp[:rows, :], AF.Square,
                                 accum_out=sq1b[m][:, c:c + 1])
            nc.vector.tensor_reduce(out=m1[m][:, 128 * c:128 * (c + 1)],
                                    in_=yp[:rows, :].rearrange("p (n k) -> p n k", k=KNN),
                                    axis=AX.X, op=ALU.max)
    b2pp.release()
    h1l.release()

    # sum(y1b) = W1b' @ sum(h1)
    sh1 = [_acc_reduce(nc, per, sumh1[m][:], rows1a[m], f"sh1r{m}") for m in range(2)]
    ps1b = tc.alloc_tile_pool(name="ps1b", bufs=1, space="PSUM")
    s1b = []
    for m, rows in enumerate(rows1b):
        msl = slice(128 * m, 128 * m + rows)
        sp = ps1b.tile([rows, 1], dt.float32, name=f"s1bp{m}", space="PSUM")
        nc.tensor.matmul(sp[:], wkt1b[0][:, msl], sh1[0][:], start=True, stop=False)
        nc.tensor.matmul(sp[:], wkt1b[1][:, msl], sh1[1][:], start=False, stop=True)
        st = per.tile([rows, 1], dt.float32, name=f"s1b_{m}")
        nc.scalar.copy(st[:], sp[:])
        s1b.append(st)
    q1b = [_acc_reduce(nc, per, sq1b[m][:], rows1b[m], f"q1br{m}") for m in range(2)]
    ps1b.release()
    ar2 = _allreduce(nc, dram, per, [s1b[0][:], s1b[1][:]], [q1b[0][:], q1b[1][:]],
                     rows1b, 181, "ar2")
    sb1b = [_bn_coeffs(nc, per, ar2[m][0][:], ar2[m][1][:], g1b[m][:], b1b[m][:],
                       rows1b[m], CNT, f"bn1b{m}") for m in range(2)]

    # x1 = prelu(bn(m1)) -> hi/lo splits + sq2, all chunk-wise
    pk2op = tc.alloc_tile_pool(name="pk2op", bufs=1)   # knn2 operands (freed w/ k2pool)
    ones53 = per.tile([53, 1], dt.float32r, name="ones53")
    nc.vector.tensor_copy(ones53[:], ones_f[:53, :])
    ones128 = per.tile([128, 1], dt.float32r, name="ones128")
    nc.vector.tensor_copy(ones128[:], ones_f[:])
    x1hi = [per.tile([r, N], dt.float32r, name=f"x1s{m}_hi") for m, r in enumerate(rows1b)]
    x1lo = [pk2op.tile([r, N], dt.bfloat16, name=f"x1s{m}_lo") for m, r in enumerate(rows1b)]
    r2h_k1 = pk2op.tile([55, N], dt.float32r, name="r2h_k1")
    psq2 = tc.alloc_tile_pool(name="psq2", bufs=1, space="PSUM")
    sqt = tc.alloc_tile_pool(name="sqt", bufs=1)
    sq2p = psq2.tile([1, N], dt.float32, name="sq2p", space="PSUM")
    for c in range(4):
        sl = slice(512 * c, 512 * (c + 1))
        first = True
        for m, rows in enumerate(rows1b):
            x1c = sqt.tile([rows, 512], dt.float32, name="x1c")
            nc.scalar.activation(x1c[:], m1[m][:, sl], AF.Prelu, bias=sb1b[m][1][:],
                                 scale=sb1b[m][0][:], alpha=0.2)
            nc.vector.tensor_copy(x1hi[m][:, sl], x1c[:])
            nc.vector.tensor_tensor(out=x1lo[m][:, sl], in0=x1c[:],
                                    in1=x1hi[m][:, sl].bitcast(dt.float32),
                                    op=ALU.subtract)
            t = sqt.tile([rows, 512], dt.float32, name="x1sqc")
            nc.vector.tensor_tensor(out=t[:], in0=x1c[:], in1=x1c[:], op=ALU.mult)
            h = sqt.tile([rows, 512], dt.float32r, name="x1sqh")
            nc.vector.tensor_copy(h[:], t[:])
            l = sqt.tile([rows, 512], dt.float32r, name="x1sql")
            nc.vector.tensor_tensor(out=l[:], in0=t[:], in1=h[:].bitcast(dt.float32),
                                    op=ALU.subtract)
            on = ones128 if rows == 128 else ones53
            nc.tensor.matmul(sq2p[:, sl], on[:], h[:], start=first, stop=False)
            first = False
            nc.tensor.matmul(sq2p[:, sl], on[:], l[:],
                             start=False, stop=(m == 1))
        sc_ = sqt.tile([1, 512], dt.float32, name="sq2c")
        nc.scalar.copy(sc_[:], sq2p[:, sl])
        sh_ = sqt.tile([1, 512], dt.float32r, name="sq2ch")
        nc.vector.tensor_copy(sh_[:], sc_[:])
        sl_ = sqt.tile([1, 512], dt.float32r, name="sq2cl")
        nc.vector.tensor_tensor(out=sl_[:], in0=sc_[:], in1=sh_[:].bitcast(dt.float32),
                                op=ALU.subtract)
        nh_ = sqt.tile([1, 512], dt.float32r, name="sq2nh")
        nc.vector.tensor_scalar_mul(nh_[:], sh_[:], -1.0)
        nc.sync.dma_start(r2h_k1[53:54, sl], nh_[:])
        nl_ = sqt.tile([1, 512], dt.float32r, name="sq2nl")
        nc.vector.tensor_scalar_mul(nl_[:], sl_[:], -1.0)
        nc.sync.dma_start(r2h_k1[54:55, sl], nl_[:])
    psq2.release()
    sqt.release()

    # knn2 operands: lhsT reuses x1hi/x1lo; only scaled rhs tiles are built
    l2h_k1 = pk2op.tile([55, N], dt.float32r, name="l2h_k1")    # x1hi 128:181 + 2 ones
    nc.vector.tensor_copy(l2h_k1[0:53, :], x1hi[1][:])
    ones2f2 = pk2op.tile([2, N], dt.float32, name="ones2f2")
    nc.vector.memset(ones2f2[:], 1.0)
    ones2n = pk2op.tile([2, N], dt.float32r, name="ones2n")
    nc.vector.tensor_copy(ones2n[:], ones2f2[:])
    nc.sync.dma_start(l2h_k1[53:55, :], ones2n[:])
    r2h_k0 = pk2op.tile([P, N], dt.float32r, name="r2h_k0")     # 2*x1hi[0]
    nc.vector.tensor_scalar_mul(r2h_k0[:], x1hi[0][:], 2.0)
    nc.vector.tensor_scalar_mul(r2h_k1[0:53, :], x1hi[1][:], 2.0)
    r2l0 = pk2op.tile([P, N], dt.bfloat16, name="r2l0")         # 2*x1lo[0]
    nc.vector.tensor_scalar_mul(r2l0[:], x1lo[0][:], 2.0)
    r2l1 = pk2op.tile([53, N], dt.bfloat16, name="r2l1")        # 2*x1lo[1]
    nc.vector.tensor_scalar_mul(r2l1[:], x1lo[1][:], 2.0)
    # bf16 copies of hi operands for the lo-correction matmuls (1 cyc/col)
    hiB0 = pk2op.tile([P, N], dt.bfloat16, name="hiB0")
    nc.vector.tensor_copy(hiB0[:], x1hi[0][:].bitcast(dt.float32))
    hiB1 = pk2op.tile([53, N], dt.bfloat16, name="hiB1")
    nc.vector.tensor_copy(hiB1[:], x1hi[1][:].bitcast(dt.float32))
    r2hB0 = pk2op.tile([P, N], dt.bfloat16, name="r2hB0")
    nc.vector.tensor_copy(r2hB0[:], r2h_k0[:].bitcast(dt.float32))
    r2hB1 = pk2op.tile([53, N], dt.bfloat16, name="r2hB1")
    nc.vector.tensor_copy(r2hB1[:], r2h_k1[0:53, :].bitcast(dt.float32))

    def mm2(pe, t, c):
        msl = slice(128 * t, 128 * (t + 1))
        csl = slice(512 * c, 512 * (c + 1))
        nc.tensor.matmul(pe[:, csl], x1hi[0][:, msl], r2h_k0[:, csl], start=True, stop=False)
        nc.tensor.matmul(pe[:, csl], l2h_k1[:, msl], r2h_k1[:, csl], start=False, stop=False)
        nc.tensor.matmul(pe[:, csl], hiB0[:, msl], r2l0[:, csl], start=False, stop=False)
        nc.tensor.matmul(pe[:, csl], hiB1[:, msl], r2l1[:, csl], start=False, stop=False)
        nc.tensor.matmul(pe[:, csl], x1lo[0][:, msl], r2hB0[:, csl], start=False, stop=False)
        nc.tensor.matmul(pe[:, csl], x1lo[1][:, msl], r2hB1[:, csl], start=False, stop=True)

    iw2 = per.tile([P, S // 16], dt.uint16, name="iw_k2")
    iw2d = per.tile([P, S // 16], dt.uint16, name="iw2d")
    knn_pass(mm2, idx2_d, "k2")
    pk2op.release()
    k2pool.release()
    ph1.release()

    # ---------------- block 2 ----------------
    # gather pools first: they reuse ph1/k2pool bytes whose users finished
    # before knn2, so the b2a gather stream can overlap knn2
    b2g = tc.alloc_tile_pool(name="b2g", bufs=1)
    gx0 = b2g.tile([P, N], dt.bfloat16, name="gx0")
    nc.scalar.copy(gx0[:], x1hi[0][:].bitcast(dt.float32))
    gx1 = b2g.tile([P, N], dt.bfloat16, name="gx1")
    nc.scalar.copy(gx1[:], gx0[:])
    nc.scalar.copy(gx1[0:53, :], x1hi[1][:].bitcast(dt.float32))
    gxi = b2g.tile([P, 2 * N], dt.bfloat16, name="gxi")
    _g3 = gxi[:].rearrange("p (n two) -> p n two", two=2)
    nc.scalar.copy(_g3[:, :, 0], gx0[:])
    nc.scalar.copy(_g3[:, :, 1], gx1[:])
    b2s = tc.alloc_tile_pool(name="b2s", bufs=3)

    # conv2b weights early so their DMA overlaps b2a / AR3
    b2bpool = tc.alloc_tile_pool(name="b2bpool", bufs=1)
    w2bT_r, w2bT_b = [], []
    for i, rows in enumerate((128, 128, 128, 46)):
        wt = wld.tile([128, 595], dt.float32, name="wf")
        nc.sync.dma_start(wt[:rows, :512], ins[f"w2bT_k{i}"])
        wr = b2bpool.tile([rows, 512], dt.float32r, name=f"w2bT_r{i}")
        nc.vector.tensor_copy(wr[:], wt[:rows, :512])
        w2bT_r.append(wr)
        wb = b2bpool.tile([rows, 512], dt.bfloat16, name=f"w2bT_b{i}")
        nc.vector.tensor_copy(wb[:], wt[:rows, :512])
        w2bT_b.append(wb)

    # U2[c, n] = sum_ci du2[ci, c] x1[ci, n]   (plain f32r)
    b2pool = tc.alloc_tile_pool(name="b2pool", bufs=1)
    w2aT_r, du2T_r = [], []
    for i, rows in enumerate((128, 53)):
        wt = wld.tile([128, 595], dt.float32, name="wf")
        nc.sync.dma_start(wt[:rows, :430], ins[f"w2aT_k{i}"])
        wr = b2pool.tile([rows, 430], dt.float32r, name=f"w2aT_r{i}")
        nc.vector.tensor_copy(wr[:], wt[:rows, :430])
        w2aT_r.append(wr)
        ddt = wld.tile([128, 595], dt.float32, name="wf")
        nc.sync.dma_start(ddt[:rows, :430], ins[f"du2T_k{i}"])
        dr = b2pool.tile([rows, 430], dt.float32r, name=f"du2T_r{i}")
        nc.vector.tensor_copy(dr[:], ddt[:rows, :430])
        du2T_r.append(dr)
    rows2a_ = (128, 128, 128, 46)
    u2 = [b2pool.tile([r, N], dt.float32, name=f"u2_{i}") for i, r in enumerate(rows2a_)]
    pu2 = tc.alloc_tile_pool(name="pu2", bufs=4, space="PSUM")
    for m, rows in enumerate(rows2a_):
        msl = slice(128 * m, 128 * m + rows)
        for cc in range(4):
            sl = slice(512 * cc, 512 * (cc + 1))
            up = pu2.tile([rows, 512], dt.float32, name="up2", space="PSUM")
            nc.tensor.matmul(up[:], du2T_r[0][:, msl], x1hi[0][:, sl], start=True, stop=False)
            nc.tensor.matmul(up[:], du2T_r[1][:, msl], x1hi[1][:, sl], start=False, stop=True)
            nc.scalar.copy(u2[m][:, sl], up[:])
    pu2.release()

    rows2a = (128, 128, 128, 46)
    y2a_d = [dram.tile([r, S], dt.bfloat16, name=f"y2a_d{i}") for i, r in enumerate(rows2a)]
    sum2a = [b2pool.tile([r, NCHUNK], dt.float32, name=f"s2a_{i}") for i, r in enumerate(rows2a)]
    sq2a = [b2pool.tile([r, NCHUNK], dt.float32, name=f"q2a_{i}") for i, r in enumerate(rows2a)]

    w2aT_b = []
    for i, rows in enumerate((128, 53)):
        wb = b2pool.tile([rows, 430], dt.bfloat16, name=f"w2aT_b{i}")
        nc.vector.tensor_copy(wb[:], w2aT_r[i][:].bitcast(dt.float32))
        w2aT_b.append(wb)

    b2p = tc.alloc_tile_pool(name="b2p", bufs=2, space="PSUM")
    b2o = tc.alloc_tile_pool(name="b2o", bufs=4)
    gxi3 = gxi[:].rearrange("p (n two) -> p n two", two=2)
    b2q = {}
    def b2_gather(c):
        cols = slice(80 * c, 80 * (c + 1))
        flat_t = idx2_d[c].rearrange("p k -> (p k)")
        nc.sync.dma_start(iw2[0:16, cols], flat_t.rearrange("(f p) -> p f", p=16))
        nc.sync.dma_start(iw2[16:32, cols], iw2[0:16, cols])
        nc.sync.dma_start(iw2[32:64, cols], iw2[0:32, cols])
        nc.sync.dma_start(iw2[64:128, cols], iw2[0:64, cols])
        nc.scalar.mul(iw2d[:, cols], iw2[:, cols], 2.0)
        nbi = b2s.tile([P, 2 * CB], dt.bfloat16, name="nbi")
        nbi3 = nbi[:].rearrange("p (n two) -> p n two", two=2)
        for g in range(4):
            nc.gpsimd.indirect_copy(
                out=nbi3[:, 320 * g:320 * (g + 1), :], data=gxi3,
                idxs=iw2d[:, 80 * c + 20 * g:80 * c + 20 * (g + 1)],
                i_know_ap_gather_is_preferred=True)
        nb0r = b2s.tile([P, CB], dt.bfloat16, name="nb0r")
        nc.vector.tensor_copy(nb0r[:], nbi3[:, :, 0])
        nb1r = b2s.tile([P, CB], dt.bfloat16, name="nb1r")
        nc.vector.tensor_copy(nb1r[:], nbi3[:, :, 1])
        b2q[c] = (nb0r, nb1r)

    for _pc in range(3):
        b2_gather(_pc)
    for c in range(NCHUNK):
        if c + 3 < NCHUNK:
            b2_gather(c + 3)
        nb0r, nb1r = b2q.pop(c)
        for m, rows in enumerate(rows2a):
            msl = slice(128 * m, 128 * m + rows)
            yp = b2p.tile([P, CB], dt.float32, name="y2ap", space="PSUM")
            for c0, c1 in CSL:
                nc.tensor.matmul(yp[:rows, c0:c1], w2aT_b[0][:, msl],
                                 nb0r[:, c0:c1], start=True, stop=False)
                nc.tensor.matmul(yp[:rows, c0:c1], w2aT_b[1][:, msl],
                                 nb1r[0:53, c0:c1], start=False, stop=True)
            uview = u2[m][:, 128 * c:128 * (c + 1), None].to_broadcast(
                [rows, 128, KNN])
            ob = b2o.tile([rows, CB], dt.bfloat16, name="ob2")
            nc.vector.scalar_tensor_tensor(
                out=ob[:].rearrange("p (n k) -> p n k", k=KNN),
                in0=yp[:rows, :].rearrange("p (n k) -> p n k", k=KNN), scalar=1.0,
                in1=uview, op0=ALU.mult, op1=ALU.add,
                accum_out=sum2a[m][:, c:c + 1])
            nc.sync.dma_start(y2a_d[m][:, CB * c:CB * (c + 1)], ob[:])
            tr = b1t.tile([rows, CB], dt.bfloat16, name="tr")
            nc.scalar.activation(tr[:], ob[:], AF.Square,
                                 accum_out=sq2a[m][:, c:c + 1])
    b2p.release()
    b2o.release()

    s2 = [_acc_reduce(nc, per, sum2a[m][:], rows2a[m], f"s2r{m}") for m in range(4)]
    q2 = [_acc_reduce(nc, per, sq2a[m][:], rows2a[m], f"q2r{m}") for m in range(4)]
    b2pool.release()

    # block-3 weights early (LIFO: pw3 below b3p/b3l/c3t/px2r); DMA overlaps conv2b
    pw3 = tc.alloc_tile_pool(name="pw3", bufs=1)
    w3aT_r = []
    rows3a_k = (128, 53, 128, 128, 128, 128)
    for i, rows in enumerate(rows3a_k):
        wt = wld.tile([128, 595], dt.float32, name="wf")
        nc.sync.dma_start(wt[:rows, :595], ins[f"w3aT_k{i}"])
        wr = pw3.tile([rows, 595], dt.bfloat16, name=f"w3aT_r{i}")
        nc.vector.tensor_copy(wr[:], wt[:rows, :595])
        w3aT_r.append(wr)
    w3bT_r = []
    rows3b_k = (128, 128, 128, 128, 83)
    for i, rows in enumerate(rows3b_k):
        wt = wld.tile([128, 595], dt.float32, name="wf")
        nc.sync.dma_start(wt[:rows, :512], ins[f"w3bT_k{i}"])
        wr = pw3.tile([rows, 512], dt.bfloat16, name=f"w3bT_r{i}")
        nc.vector.tensor_copy(wr[:], wt[:rows, :512])
        w3bT_r.append(wr)
    pb3 = tc.alloc_tile_pool(name="pb3", bufs=1)   # y3a/y3b bf16 residency

    rows2b = (128, 128, 128, 128)
    sumh2 = [b2bpool.tile([r, NCHUNK], dt.float32, name=f"sh2_{i}") for i, r in enumerate(rows2a)]
    sq2b = [b2bpool.tile([r, NCHUNK], dt.float32, name=f"q2b_{i}") for i, r in enumerate(rows2b)]
    m2 = [b2bpool.tile([r, N], dt.bfloat16, name=f"m2_{i}") for i, r in enumerate(rows2b)]

    b3p = tc.alloc_tile_pool(name="b3p", bufs=2, space="PSUM")
    b3l = tc.alloc_tile_pool(name="b3l", bufs=2)

    c2q = {}
    def c2_load(c):
        csl = slice(CB * c, CB * (c + 1))
        grp = []
        for m, rows in enumerate(rows2a):
            ld = b3l.tile([rows, CB], dt.bfloat16, name=f"ld{m}")
            nc.sync.dma_start(ld[:], y2a_d[m][:, csl])
            grp.append(ld)
        c2q[c] = grp

    c2_load(0)
    ar3 = _allreduce(nc, dram, per, [s[:] for s in s2], [q[:] for q in q2],
                     rows2a, 430, "ar3")
    sb2a = [_bn_coeffs(nc, per, ar3[m][0][:], ar3[m][1][:], g2a[m][:], b2a[m][:],
                       rows2a[m], CNT, f"bn2a{m}") for m in range(4)]
    for c in range(NCHUNK):
        if c + 1 < NCHUNK:
            c2_load(c + 1)
        csl = slice(CB * c, CB * (c + 1))
        h2 = c2q.pop(c)
        for m, rows in enumerate(rows2a):
            ld = h2[m]
            nc.scalar.activation(ld[:], ld[:], AF.Prelu, bias=sb2a[m][1][:],
                                 scale=sb2a[m][0][:], alpha=0.2,
                                 accum_out=sumh2[m][:, c:c + 1])
        for m, rows in enumerate(rows2b):
            msl = slice(128 * m, 128 * (m + 1))
            yp = b3p.tile([P, CB], dt.float32, name="y2bp", space="PSUM")
            for c0, c1 in CSL:
                for k in range(4):
                    kr = (128, 128, 128, 46)[k]
                    nc.tensor.matmul(yp[:rows, c0:c1], w2bT_b[k][:, msl],
                                     h2[k][0:kr, c0:c1],
                                     start=(k == 0), stop=(k == 3))
            tr = b1t.tile([rows, CB], dt.bfloat16, name="tr")
            nc.scalar.activation(tr[:], yp[:rows, :], AF.Square,
                                 accum_out=sq2b[m][:, c:c + 1])
            nc.vector.tensor_reduce(out=m2[m][:, 128 * c:128 * (c + 1)],
                                    in_=yp[:rows, :].rearrange("p (n k) -> p n k", k=KNN),
                                    axis=AX.X, op=ALU.max)
    b3p.release()
    b3l.release()

    sh2 = [_acc_reduce(nc, per, sumh2[m][:], rows2a[m], f"sh2r{m}") for m in range(4)]
    sh2r = []
    for m in range(4):
        t = per.tile([rows2a[m], 4], dt.float32r, name=f"sh2rr{m}")
        nc.vector.tensor_copy(t[:], sh2[m][:].to_broadcast([rows2a[m], 4]))
        sh2r.append(t)
    ps2b = tc.alloc_tile_pool(name="ps2b", bufs=1, space="PSUM")
    s2b = []
    for m in range(4):
        msl = slice(128 * m, 128 * (m + 1))
        sp = ps2b.tile([128, 4], dt.float32, name=f"s2bp{m}", space="PSUM")
        for k in range(4):
            kr = (128, 128, 128, 46)[k]
            nc.tensor.matmul(sp[:], w2bT_r[k][:, msl], sh2r[k][0:kr, :],
                             start=(k == 0), stop=(k == 3))
        st = per.tile([128, 1], dt.float32, name=f"s2b_{m}")
        nc.scalar.copy(st[:], sp[:, 0:1])
        s2b.append(st)
    ps2b.release()
    q2b = [_acc_reduce(nc, per, sq2b[m][:], 128, f"q2br{m}") for m in range(4)]
    ar4 = _allreduce(nc, dram, per, [s[:] for s in s2b], [q[:] for q in q2b],
                     rows2b, 512, "ar4")
    sb2b = [_bn_coeffs(nc, per, ar4[m][0][:], ar4[m][1][:], g2b[m][:], b2b[m][:],
                       128, CNT, f"bn2b{m}") for m in range(4)]

    # x2 = prelu(bn(m2)) -> bf16; x1 recast to bf16 for block 3
    c3t = tc.alloc_tile_pool(name="c3t", bufs=4)
    px2r = tc.alloc_tile_pool(name="px2r", bufs=1)
    x2r = [px2r.tile([128, N], dt.bfloat16, name=f"x2r_{i}") for i in range(4)]
    for m in range(4):
        nc.scalar.activation(x2r[m][:], m2[m][:], AF.Prelu, bias=sb2b[m][1][:],
                             scale=sb2b[m][0][:], alpha=0.2)
    xb1 = [px2r.tile([r, N], dt.bfloat16, name=f"xb1_{i}")
           for i, r in enumerate((128, 53))]
    for i in range(2):
        nc.vector.tensor_copy(xb1[i][:], x1hi[i][:].bitcast(dt.float32))

    # ---------------- block 3 ----------------
    # y3a / y3b stay SBUF-resident in bf16 (post-x1 path tolerates bf16)
    xc_k = [xb1[0], xb1[1], x2r[0], x2r[1], x2r[2], x2r[3]]
    rows3a = (128, 128, 128, 128, 83)
    y3a_s = [pb3.tile([r, N], dt.bfloat16, name=f"y3a_s{i}") for i, r in enumerate(rows3a)]
    sum3a = [per.tile([r, 4], dt.float32, name=f"s3a_{i}") for i, r in enumerate(rows3a)]
    sq3a = [per.tile([r, 4], dt.float32, name=f"q3a_{i}") for i, r in enumerate(rows3a)]
    c3p = tc.alloc_tile_pool(name="c3p", bufs=4, space="PSUM")
    for c in range(4):
        csl = slice(512 * c, 512 * (c + 1))
        for m, rows in enumerate(rows3a):
            msl = slice(128 * m, 128 * m + rows)
            yp = c3p.tile([128, 512], dt.float32, name="y3ap", space="PSUM")
            for k in range(6):
                nc.tensor.matmul(yp[:rows, :], w3aT_r[k][:, msl], xc_k[k][:, csl],
                                 start=(k == 0), stop=(k == 5))
            nc.scalar.activation(y3a_s[m][:, csl], yp[:rows, :], AF.Copy,
                                 accum_out=sum3a[m][:, c:c + 1])
            tr = c3t.tile([rows, 512], dt.bfloat16, name="tr")
            nc.scalar.activation(tr[:], yp[:rows, :], AF.Square,
                                 accum_out=sq3a[m][:, c:c + 1])
    c3p.release()
    px2r.release()

    s3 = [_acc_reduce(nc, per, sum3a[m][:], rows3a[m], f"s3r{m}") for m in range(5)]
    q3 = [_acc_reduce(nc, per, sq3a[m][:], rows3a[m], f"q3r{m}") for m in range(5)]
    ar5 = _allreduce(nc, dram, per, [s[:] for s in s3], [q[:] for q in q3],
                     rows3a, 640, "ar5")
    sb3a = [_bn_coeffs(nc, per, ar5[m][0][:], ar5[m][1][:], g3a[m][:], b3a[m][:],
                       rows3a[m], CNT3, f"bn3a{m}") for m in range(5)]

    rows3b = (128, 128, 128, 128)
    y3b_s = [pb3.tile([r, N], dt.bfloat16, name=f"y3b_s{i}") for i, r in enumerate(rows3b)]
    sum3b = [per.tile([r, 4], dt.float32, name=f"s3b_{i}") for i, r in enumerate(rows3b)]
    sq3b = [per.tile([r, 4], dt.float32, name=f"q3b_{i}") for i, r in enumerate(rows3b)]
    c4p = tc.alloc_tile_pool(name="c4p", bufs=2, space="PSUM")
    h3l = tc.alloc_tile_pool(name="h3l", bufs=10)
    for c in range(4):
        csl = slice(512 * c, 512 * (c + 1))
        h3c = []
        for k, kr in enumerate(rows3a):
            ld = h3l.tile([kr, 512], dt.bfloat16, name="h3ld")
            nc.scalar.activation(ld[:], y3a_s[k][:, csl], AF.Prelu,
                                 bias=sb3a[k][1][:], scale=sb3a[k][0][:], alpha=0.2)
            h3c.append(ld)
        for m, rows in enumerate(rows3b):
            msl = slice(128 * m, 128 * (m + 1))
            yp = c4p.tile([rows, 512], dt.float32, name=f"y3bp{m}", space="PSUM")
            for k in range(5):
                nc.tensor.matmul(yp[:], w3bT_r[k][:, msl], h3c[k][:],
                                 start=(k == 0), stop=(k == 4))
            nc.scalar.activation(y3b_s[m][:, csl], yp[:], AF.Copy,
                                 accum_out=sum3b[m][:, c:c + 1])
            tr = c3t.tile([rows, 512], dt.bfloat16, name="tr")
            nc.scalar.activation(tr[:], yp[:], AF.Square,
                                 accum_out=sq3b[m][:, c:c + 1])
    c4p.release()
    h3l.release()
    c3t.release()

    s4 = [_acc_reduce(nc, per, sum3b[m][:], 128, f"s4r{m}") for m in range(4)]
    q4 = [_acc_reduce(nc, per, sq3b[m][:], 128, f"q4r{m}") for m in range(4)]
    ar6 = _allreduce(nc, dram, per, [s[:] for s in s4], [q[:] for q in q4],
                     rows3b, 512, "ar6")
    sb3b = [_bn_coeffs(nc, per, ar6[m][0][:], ar6[m][1][:], g3b[m][:], b3b[m][:],
                       128, CNT3, f"bn3b{m}") for m in range(4)]

    out_d = nc._ext_outputs["out"]
    for m in range(4):
        for c in range(4):
            csl = slice(512 * c, 512 * (c + 1))
            ot = oseg.tile([128, 512], dt.float32, name="ot")
            nc.scalar.activation(ot[:], y3b_s[m][:, csl], AF.Prelu,
                                 bias=sb3b[m][1][:], scale=sb3b[m][0][:], alpha=0.2)
            nc.sync.dma_start(out_d[128 * m:128 * (m + 1), csl], ot[:])
    pb3.release()
    pw3.release()
    b2bpool.release()
    b2s.release()
    b2g.release()
    wld.release()
    oseg.release()
    b1t.release()
    per.release()


# ------------------------------------------------------------------ host side
_CACHE = {}


def _build():
    _install_bassfix()
    nc = bass.Bass("TRN2", target_bir_lowering=False, debug=False,
                   num_devices=NCORES)
    in_specs = {
        "x": (64, N), "w1aT": (64, 152), "du1T": (64, 152),
        "w1bT_k0": (128, 181), "w1bT_k1": (24, 181),
        "w2aT_k0": (128, 430), "w2aT_k1": (53, 430),
        "du2T_k0": (128, 430), "du2T_k1": (53, 430),
        "w2bT_k0": (128, 512), "w2bT_k1": (128, 512),
        "w2bT_k2": (128, 512), "w2bT_k3": (46, 512),
        "w3aT_k0": (128, 595), "w3aT_k1": (53, 595), "w3aT_k2": (128, 595),
        "w3aT_k3": (128, 595), "w3aT_k4": (128, 595), "w3aT_k5": (128, 595),
        "w3bT_k0": (128, 512), "w3bT_k1": (128, 512), "w3bT_k2": (128, 512),
        "w3bT_k3": (128, 512), "w3bT_k4": (83, 512),
        "g1a": (152, 1), "be1a": (152, 1), "g1b": (181, 1), "be1b": (181, 1),
        "g2a": (430, 1), "be2a": (430, 1), "g2b": (512, 1), "be2b": (512, 1),
        "g3a": (595, 1), "be3a": (595, 1), "g3b": (512, 1), "be3b": (512, 1),
        "E": (32, CB),
    }
    nc._ext_inputs = {}
    for nm, shp in in_specs.items():
        nc._ext_inputs[nm] = nc.dram_tensor(nm, list(shp), dt.float32,
                                            kind="ExternalInput").ap()
    nc._ext_outputs = {
        "out": nc.dram_tensor("out", [512, N], dt.float32,
                              kind="ExternalOutput").ap()}
    with tile.TileContext(nc) as tc:
        build_kernel(nc, tc)
    return nc


def _host_inputs(x, c1w1, c1g1, c1be1, c1w2, c1g2, c1be2,
                 c2w1, c2g1, c2be1, c2w2, c2g2, c2be2,
                 c3w1, c3g1, c3be1, c3w2, c3g2, c3be2):
    f32 = np.float32
    W1 = np.asarray(c1w1, f32)            # [152, 128]
    W1a, W1b = W1[:, :64], W1[:, 64:]
    W2 = np.asarray(c2w1, f32)            # [430, 362]
    W2a, W2b = W2[:, :181], W2[:, 181:]
    ws = {
        "w1aT": np.ascontiguousarray(W1a.T),
        "du1T": np.ascontiguousarray((W1b - W1a).T),
    }
    w1bT = np.ascontiguousarray(np.asarray(c1w2, f32).T)     # [152, 181]
    ws["w1bT_k0"], ws["w1bT_k1"] = w1bT[:128], w1bT[128:]
    w2aT = np.ascontiguousarray(W2a.T)                        # [181, 430]
    du2T = np.ascontiguousarray((W2b - W2a).T)
    ws["w2aT_k0"], ws["w2aT_k1"] = w2aT[:128], w2aT[128:]
    ws["du2T_k0"], ws["du2T_k1"] = du2T[:128], du2T[128:]
    w2bT = np.ascontiguousarray(np.asarray(c2w2, f32).T)     # [430, 512]
    for i, sl in enumerate((slice(0, 128), slice(128, 256), slice(256, 384),
                            slice(384, 430))):
        ws[f"w2bT_k{i}"] = w2bT[sl]
    w3aT = np.ascontiguousarray(np.asarray(c3w1, f32).T)     # [693, 595]
    cuts = (0, 128, 181, 309, 437, 565, 693)
    for i in range(6):
        ws[f"w3aT_k{i}"] = w3aT[cuts[i]:cuts[i + 1]]
    w3bT = np.ascontiguousarray(np.asarray(c3w2, f32).T)     # [595, 512]
    for i in range(5):
        ws[f"w3bT_k{i}"] = w3bT[128 * i:min(128 * (i + 1), 595)]
    for nm, v in (("g1a", c1g1), ("be1a", c1be1), ("g1b", c1g2), ("be1b", c1be2),
                  ("g2a", c2g1), ("be2a", c2be1), ("g2b", c2g2), ("be2b", c2be2),
                  ("g3a", c3g1), ("be3a", c3be1), ("g3b", c3g2), ("be3b", c3be2)):
        ws[nm] = np.ascontiguousarray(np.asarray(v, f32).reshape(-1, 1))
    E = np.zeros((32, CB), f32)
    for r in range(32):
        E[r, KNN * r:KNN * (r + 1)] = 1.0
    ws["E"] = E
    ws = {k: np.ascontiguousarray(v, f32) for k, v in ws.items()}
    xs = np.asarray(x, f32)
    in_maps = []
    for i in range(NCORES):
        m = dict(ws)
        m["x"] = np.ascontiguousarray(xs[i])
        in_maps.append(m)
    return in_maps


def kernel(x, c1w1, c1b1, c1g1, c1be1, c1w2, c1b2, c1g2, c1be2,
           c2w1, c2b1, c2g1, c2be1, c2w2, c2b2, c2g2, c2be2,
           c3w1, c3b1, c3g1, c3be1, c3w2, c3b2, c3g2, c3be2):
    # conv biases are absorbed exactly by the following BatchNorm (shift
    # invariance), so b* inputs are unused by the device program.
    if "nc" not in _CACHE:
        _CACHE["nc"] = _build()
    nc = _CACHE["nc"]
    in_maps = _host_inputs(x, c1w1, c1g1, c1be1, c1w2, c1g2, c1be2,
                           c2w1, c2g1, c2be1, c2w2, c2g2, c2be2,
                           c3w1, c3g1, c3be1, c3w2, c3g2, c3be2)
    res = run_bass_kernel_spmd(nc, in_maps, core_ids=list(range(NCORES)))
    _CACHE["last_result"] = res
    out = np.stack([res.results[i]["out"] for i in range(NCORES)], axis=0)
    return out.astype(np.float32)



# revision 42
# speedup vs baseline: 1.1626x; 1.0040x over previous
"""DGCNN-style GCN encoder on 8 TRN2 NeuronCores (bass/tile).

Data-parallel over batch: each core owns one sample (B=8). BatchNorm batch
statistics are all-reduced across the 8 cores so results match global-batch
BN exactly. kNN top-10, neighbor gathers, and all convs/BN/activations run
on-device; the host only reshapes/replicates weights.

Precision strategy: matmuls on the x -> x1 -> knn2 path use a compensated
fp32r (hi/lo split) scheme that recovers ~fp32 accuracy at fp32r speed
(conv1b uses plain fp32). This keeps the kNN neighbor selection stable
against the f32 reference. Everything downstream of x1 (conv2a/2b, block 3)
runs in bf16 (weights + activations), which the 2e-2 tolerance absorbs.

Performance notes (vs the first working version):
- edge loops process 1280-edge chunks (128 points x 10 nbrs) instead of 320
- neighbor gathers are indirect_copy, whose cost is ~28ns per index of Q7
  dispatch; conv2a gathers both x1 row-tiles in ONE pass by interleaving
  them as bf16 pairs (d=2 gather with doubled indices), halving index count
- gathers/loads are software-pipelined one-to-two chunks ahead, and the
  next phase's first loads + weight loads are issued before each BN
  AllReduce so DMA overlaps the collective
- block-1's gather stream overlaps knn1 (per-tile iw builds + pools
  pre-allocated before the knn pass so gather t only waits on top-k tile t);
  iw2/iw2d are likewise built per-tile during knn2
- knn2's lo-correction matmuls (4 of 6) run in bf16 at 1 cyc/col; only the
  two hi*hi + sq-fold matmuls stay compensated f32r, keeping neighbor
  selection error ~5e-5 absolute
- y3a/y3b stay SBUF-resident in bf16 (no DRAM round-trip); y2a is spilled
  to DRAM in bf16 (half the traffic)
"""

import os
import sys
import json

B16 = "/nix/store/wxap7svlj45h0lfm31d1axjjnzyl6qsy-b16-bazel-unstable-cc-2026-05-04-9a3fa1f3-rt-2026-05-04-ade39e0a/lib/python3.13/site-packages"
if B16 not in sys.path:
    sys.path.insert(0, B16)
if "/opt/trn_rl_repo" not in sys.path:
    sys.path.insert(0, "/opt/trn_rl_repo")

import numpy as np
import concourse.bass as bass
import concourse.mybir as mybir
import concourse.tile as tile
from concourse.bass_utils import run_bass_kernel_spmd

dt = mybir.dt
AF = mybir.ActivationFunctionType
ALU = mybir.AluOpType
AX = mybir.AxisListType

N = 2048
KNN = 10
S = N * KNN            # 20480 edge positions
CB = 1280              # banded chunk: 128 points x 10 neighbors
NCHUNK = S // CB       # 16
CSL = [(0, 512), (512, 1024), (1024, 1280)]   # matmul col slices of a chunk
EPS = 1e-5
NCORES = 8
CNT = float(NCORES * S)      # BN count for edge tensors
CNT3 = float(NCORES * N)     # BN count for block-3 tensors


# ---------------------------------------------------------------- walrus fix
def _fix_bir_json(bir_json: bytes) -> bytes:
    """This container's walrus accepts only ONE sync-wait per instruction;
    hoist extra waits onto preceding EventSemaphore instructions."""
    m = json.loads(bir_json)
    for f in m["functions"]:
        for bb in f["blocks"]:
            out = []
            for i in bb["instructions"]:
                si = i.get("sync_info") or {}
                waits = si.get("on_wait") or []
                if len(waits) > 1:
                    for k, w in enumerate(waits[:-1]):
                        out.append({
                            "name": f"{i['name']}w{k}",
                            "engine": i["engine"],
                            "opcode": "EventSemaphore",
                            "ins": [], "outs": [],
                            "debug": i.get("debug"),
                            "sync_info": {"on_update": [], "on_wait": [w]},
                        })
                    i = dict(i)
                    si = dict(si)
                    si["on_wait"] = [waits[-1]]
                    i["sync_info"] = si
                out.append(i)
            bb["instructions"] = out
    return json.dumps(m).encode()


def _install_bassfix():
    import concourse.bass_utils as bu
    import concourse.bass2jax as b2j
    if getattr(bu, "_bassfix_installed", False):
        return
    real = bu.compile_bir_kernel

    def patched(bir_json, tmpdir, neff_name="file.neff"):
        return real(_fix_bir_json(bir_json), tmpdir, neff_name)

    bu.compile_bir_kernel = patched
    b2j.compile_bir_kernel = patched
    bu._bassfix_installed = True


# ------------------------------------------------------------- device helpers
def _round_split(nc, pool, src_ap, parts, width, name):
    """Return (hi, lo) float32r tiles for a f32 source AP [parts, width]."""
    hi = pool.tile([parts, width], dt.float32r, name=name + "_hi")
    lo = pool.tile([parts, width], dt.float32r, name=name + "_lo")
    nc.vector.tensor_copy(hi[:], src_ap)
    nc.vector.tensor_tensor(out=lo[:], in0=src_ap, in1=hi[:].bitcast(dt.float32),
                            op=ALU.subtract)
    return hi, lo


def _bn_coeffs(nc, pool, sums, sqs, g_ap, b_ap, rows, cnt, name):
    """sums/sqs: [rows,1] f32 (global). Returns (scale, bias) [rows,1] f32."""
    mean = pool.tile([rows, 1], dt.float32, name=name + "_mean")
    nc.vector.tensor_scalar_mul(mean[:], sums, 1.0 / cnt)
    e2 = pool.tile([rows, 1], dt.float32, name=name + "_e2")
    nc.vector.tensor_scalar_mul(e2[:], sqs, 1.0 / cnt)
    m2 = pool.tile([rows, 1], dt.float32, name=name + "_m2")
    nc.vector.tensor_tensor(out=m2[:], in0=mean[:], in1=mean[:], op=ALU.mult)
    var = pool.tile([rows, 1], dt.float32, name=name + "_var")
    nc.vector.tensor_tensor(out=var[:], in0=e2[:], in1=m2[:], op=ALU.subtract)
    epst = pool.tile([rows, 1], dt.float32, name=name + "_eps")
    nc.vector.memset(epst[:], float(EPS))
    std = pool.tile([rows, 1], dt.float32, name=name + "_std")
    nc.scalar.activation(std[:], var[:], AF.Sqrt, bias=epst[:])
    rstd = pool.tile([rows, 1], dt.float32, name=name + "_rstd")
    nc.vector.reciprocal(rstd[:], std[:])
    scale = pool.tile([rows, 1], dt.float32, name=name + "_scale")
    nc.vector.tensor_tensor(out=scale[:], in0=rstd[:], in1=g_ap, op=ALU.mult)
    ms = pool.tile([rows, 1], dt.float32, name=name + "_ms")
    nc.vector.tensor_tensor(out=ms[:], in0=mean[:], in1=scale[:], op=ALU.mult)
    bias = pool.tile([rows, 1], dt.float32, name=name + "_bias")
    nc.vector.tensor_tensor(out=bias[:], in0=b_ap, in1=ms[:], op=ALU.subtract)
    return scale, bias


def _allreduce(nc, dram, per, sums_list, sqs_list, rows_list, cpad, name):
    """Pack per-Mtile [rows,1] sum/sq into a DRAM [cpad,2] buffer, AllReduce
    across the 8 cores, unpack back into fresh [rows,1] tiles."""
    cin = dram.tile([cpad, 2], dt.float32, name=name + "_in")
    cout = dram.tile([cpad, 2], dt.float32, name=name + "_out")
    r0 = 0
    for s_, q_, rows in zip(sums_list, sqs_list, rows_list):
        nc.sync.dma_start(cin[r0:r0 + rows, 0:1], s_)
        nc.sync.dma_start(cin[r0:r0 + rows, 1:2], q_)
        r0 += rows
    if r0 < cpad:
        z = per.tile([cpad - r0, 2], dt.float32, name=name + "_z")
        nc.vector.memset(z[:], 0.0)
        nc.sync.dma_start(cin[r0:cpad, :], z[:])
    nc.gpsimd.collective_compute(
        "AllReduce", ALU.add, replica_groups=[list(range(NCORES))],
        ins=[cin[:].opt()], outs=[cout[:].opt()])
    outs = []
    r0 = 0
    for rows in rows_list:
        gs = per.tile([rows, 1], dt.float32, name=name + f"_gs{r0}")
        gq = per.tile([rows, 1], dt.float32, name=name + f"_gq{r0}")
        nc.sync.dma_start(gs[:], cout[r0:r0 + rows, 0:1])
        nc.sync.dma_start(gq[:], cout[r0:r0 + rows, 1:2])
        outs.append((gs, gq))
        r0 += rows
    return outs


def _acc_reduce(nc, per, acc, rows, name):
    """[rows, NCHUNK] accumulator -> [rows,1]."""
    out = per.tile([rows, 1], dt.float32, name=name)
    nc.vector.tensor_reduce(out=out[:], in_=acc, axis=AX.X, op=ALU.add)
    return out


def build_kernel(nc, tc):
    P = 128
    per = tc.alloc_tile_pool(name="persist", bufs=1)
    dram = tc.alloc_tile_pool(name="dram", bufs=1, space="DRAM")
    b1t = tc.alloc_tile_pool(name="b1t", bufs=2)
    oseg = tc.alloc_tile_pool(name="oseg", bufs=1)
    wld = tc.alloc_tile_pool(name="wld", bufs=2)
    ph1 = tc.alloc_tile_pool(name="ph1", bufs=1)

    ins = nc._ext_inputs
    x_d = ins["x"]

    # ---------------- load & prep ----------------
    tmp0 = tc.alloc_tile_pool(name="tmp0", bufs=1)      # until U1t/rhs built
    x = tmp0.tile([64, N], dt.float32, name="x")
    nc.sync.dma_start(x[:], x_d)
    xhi, xlo = _round_split(nc, tmp0, x[:], 64, N, "x")
    # stacks for compensated matmuls
    xstack = ph1.tile([P, N], dt.float32r, name="xstack")   # [xhi; xlo]
    nc.vector.tensor_copy(xstack[:64, :], xhi[:])
    nc.vector.tensor_copy(xstack[64:, :], xlo[:])
    lhsx = xstack                                             # [xhi; xlo]
    rhsx = ph1.tile([P, N], dt.float32r, name="rhsx")        # 2*[xlo; xhi]
    nc.vector.tensor_scalar_mul(rhsx[:64, :], xlo[:], 2.0)
    nc.vector.tensor_scalar_mul(rhsx[64:, :], xhi[:], 2.0)

    # sq1 = sum_c x^2 (compensated)
    xsq = tmp0.tile([64, N], dt.float32, name="xsq")
    nc.vector.tensor_tensor(out=xsq[:], in0=x[:], in1=x[:], op=ALU.mult)
    xsqhi, xsqlo = _round_split(nc, tmp0, xsq[:], 64, N, "xsq")
    ones_f = per.tile([128, 1], dt.float32, name="ones_f")
    nc.vector.memset(ones_f[:], 1.0)
    ones64 = per.tile([64, 1], dt.float32r, name="ones64")
    nc.vector.tensor_copy(ones64[:], ones_f[:64, :])
    # knn1 matmul operands (sq1 folded in chunk-wise)
    lhs_hi1 = ph1.tile([66, N], dt.float32r, name="lhs_hi1")
    nc.vector.tensor_copy(lhs_hi1[:64, :], xhi[:])
    ones2f = tmp0.tile([2, N], dt.float32, name="ones2f")
    nc.vector.memset(ones2f[:], 1.0)
    nc.vector.tensor_copy(lhs_hi1[64:66, :], ones2f[:])
    rhs_hi1 = ph1.tile([66, N], dt.float32r, name="rhs_hi1")
    nc.vector.tensor_scalar_mul(rhs_hi1[:64, :], xhi[:], 2.0)
    psq = tc.alloc_tile_pool(name="psq", bufs=1, space="PSUM")
    sq1p = psq.tile([1, N], dt.float32, name="sq1p", space="PSUM")
    for c in range(4):
        sl = slice(512 * c, 512 * (c + 1))
        nc.tensor.matmul(sq1p[:, sl], ones64[:], xsqhi[:, sl], start=True, stop=False)
        nc.tensor.matmul(sq1p[:, sl], ones64[:], xsqlo[:, sl], start=False, stop=True)
        sc_ = tmp0.tile([1, 512], dt.float32, name="sq1c")
        nc.scalar.copy(sc_[:], sq1p[:, sl])
        sh_ = tmp0.tile([1, 512], dt.float32r, name="sq1ch")
        nc.vector.tensor_copy(sh_[:], sc_[:])
        sl_ = tmp0.tile([1, 512], dt.float32r, name="sq1cl")
        nc.vector.tensor_tensor(out=sl_[:], in0=sc_[:], in1=sh_[:].bitcast(dt.float32),
                                op=ALU.subtract)
        nc.vector.tensor_scalar_mul(rhs_hi1[64:65, sl], sh_[:], -1.0)
        nl_ = tmp0.tile([1, 512], dt.float32r, name="sq1nl")
        nc.vector.tensor_scalar_mul(nl_[:], sl_[:], -1.0)
        nc.sync.dma_start(rhs_hi1[65:66, sl], nl_[:])
    psq.release()

    # conv1 weights
    w1aT = wld.tile([128, 595], dt.float32, name="wf")
    nc.sync.dma_start(w1aT[:64, :152], ins["w1aT"])
    w1aT_hi, w1aT_lo = _round_split(nc, per, w1aT[:64, :152], 64, 152, "w1aT")
    du1T = wld.tile([128, 595], dt.float32, name="wf")
    nc.sync.dma_start(du1T[:64, :152], ins["du1T"])
    du1T_hi, du1T_lo = _round_split(nc, per, du1T[:64, :152], 64, 152, "du1T")
    w1aX = per.tile([P, 152], dt.float32r, name="w1aX")      # [Wlo; Whi]
    nc.vector.tensor_copy(w1aX[:64, :], w1aT_lo[:])
    nc.vector.tensor_copy(w1aX[64:, :], w1aT_hi[:])
    du1X = per.tile([P, 152], dt.float32r, name="du1X")      # [dlo; dhi] pairs lhsx
    nc.vector.tensor_copy(du1X[:64, :], du1T_lo[:])
    nc.vector.tensor_copy(du1X[64:, :], du1T_hi[:])

    wkt1b = []  # conv1b fp32 weights, K-tiles
    for i, rows in enumerate((128, 24)):
        t = per.tile([rows, 181], dt.float32, name=f"w1bT_k{i}")
        nc.sync.dma_start(t[:], ins[f"w1bT_k{i}"])
        wkt1b.append(t)


    # U1[c, n] = sum_ci du1[ci, c] x[ci, n]   (compensated, natural layout)
    rows1a_ = (128, 24)
    u1 = [ph1.tile([r, N], dt.float32, name=f"u1_{i}") for i, r in enumerate(rows1a_)]
    pu = tc.alloc_tile_pool(name="pu", bufs=4, space="PSUM")
    for m, rows in enumerate(rows1a_):
        msl = slice(128 * m, 128 * m + rows)
        for cc in range(4):
            sl = slice(512 * cc, 512 * (cc + 1))
            up = pu.tile([rows, 512], dt.float32, name="up", space="PSUM")
            nc.tensor.matmul(up[:], du1T_hi[:, msl], xhi[:, sl], start=True, stop=False)
            nc.tensor.matmul(up[:], du1X[:, msl], xstack[:, sl], start=False, stop=True)
            nc.scalar.copy(u1[m][:, sl], up[:])
    pu.release()
    tmp0.release()

    # BN params
    def load_vec(nm, rows_list):
        tiles = []
        r0 = 0
        for i, rows in enumerate(rows_list):
            t = per.tile([rows, 1], dt.float32, name=f"{nm}_{i}")
            nc.sync.dma_start(t[:], ins[nm][r0:r0 + rows, :])
            tiles.append(t)
            r0 += rows
        return tiles

    g1a = load_vec("g1a", (128, 24)); b1a = load_vec("be1a", (128, 24))
    g1b = load_vec("g1b", (128, 53)); b1b = load_vec("be1b", (128, 53))
    g2a = load_vec("g2a", (128, 128, 128, 46)); b2a = load_vec("be2a", (128, 128, 128, 46))
    g2b = load_vec("g2b", (128,) * 4); b2b = load_vec("be2b", (128,) * 4)
    g3a = load_vec("g3a", (128,) * 4 + (83,)); b3a = load_vec("be3a", (128,) * 4 + (83,))
    g3b = load_vec("g3b", (128,) * 4); b3b = load_vec("be3b", (128,) * 4)

    # ---------------- kNN (shared machinery) ----------------
    idx1_d = dram.tile([16, P, KNN], dt.uint16, name="idx1_d")
    idx2_d = dram.tile([16, P, KNN], dt.uint16, name="idx2_d")

    def knn_pass(mm_emit, idx_dram, tag, after_tile=None, kp_bufs=2):
        """mm_emit(psum_slice, c) emits matmuls for columns 512c..512c+512."""
        kp = tc.alloc_tile_pool(name=f"knnp_{tag}", bufs=kp_bufs, space="PSUM")
        ks = tc.alloc_tile_pool(name=f"knns_{tag}", bufs=2)
        for t in range(16):
            pe = kp.tile([P, N], dt.float32, name="pe", space="PSUM")
            for c in range(4):
                mm_emit(pe, t, c)
            q = ks.tile([P, N], dt.float32, name="q")
            nc.scalar.copy(q[:], pe[:])
            v8 = ks.tile([P, 8], dt.float32, name="v8")
            i8 = ks.tile([P, 8], dt.uint16, name="i8")
            nc.vector.max(out=v8[:], in_=q[:])
            nc.vector.max_index(out=i8[:], in_max=v8[:], in_values=q[:])
            nc.vector.match_replace(out=q[:], in_to_replace=v8[:], in_values=q[:],
                                    imm_value=-1e30)
            v8b = ks.tile([P, 8], dt.float32, name="v8b")
            i8b = ks.tile([P, 8], dt.uint16, name="i8b")
            nc.vector.max(out=v8b[:], in_=q[:])
            nc.vector.max_index(out=i8b[:], in_max=v8b[:], in_values=q[:])
            idx = ks.tile([P, KNN], dt.uint16, name="idx")
            nc.vector.tensor_copy(idx[:, 0:7], i8[:, 1:8])
            nc.vector.tensor_copy(idx[:, 7:10], i8b[:, 0:3])
            nc.sync.dma_start(idx_dram[t], idx[:])
            if after_tile is not None:
                after_tile(t)
        kp.release()
        ks.release()

    def build_wrapped(idx_dram, tag):
        """DRAM flat idx (point-major) -> wrapped [128, S/16] uint16."""
        iw = per.tile([P, S // 16], dt.uint16, name=f"iw_{tag}")
        flat = idx_dram[:].rearrange("a p k -> (a p k)")
        nc.sync.dma_start(iw[0:16, :], flat.rearrange("(f p) -> p f", p=16))
        for g in range(1, 8):
            nc.sync.dma_start(iw[16 * g:16 * (g + 1), :], iw[0:16, :])
        return iw

    # knn1 matmuls
    def mm1(pe, t, c):
        msl = slice(128 * t, 128 * (t + 1))
        csl = slice(512 * c, 512 * (c + 1))
        nc.tensor.matmul(pe[:, csl], lhs_hi1[:, msl], rhs_hi1[:, csl],
                         start=True, stop=False)
        nc.tensor.matmul(pe[:, csl], lhsx[:, msl], rhsx[:, csl],
                         start=False, stop=True)

    # ---------------- block 1, interleaved with knn1 ----------------
    # y1a[(n,j)] = W1a @ (nb - cen) + W1b @ cen ; stats; keep for BN.
    # Each knn1 tile's top-k feeds its 4 stats chunks immediately, so the
    # gather-paced loop overlaps the DVE top-k of later tiles.
    y1a_d = [dram.tile([P, S], dt.float32, name="y1a_d0"),
             dram.tile([24, S], dt.float32, name="y1a_d1")]
    rows1a = (128, 24)
    sum1a = [ph1.tile([r, NCHUNK], dt.float32, name=f"s1a_{i}") for i, r in enumerate(rows1a)]
    sq1a = [ph1.tile([r, NCHUNK], dt.float32, name=f"q1a_{i}") for i, r in enumerate(rows1a)]
    # b1 pools pre-allocated so the gather stream overlaps knn1; iw1 built
    # per-tile so gather t only depends on top-k tile t
    iw1 = per.tile([P, S // 16], dt.uint16, name="iw_k1")
    b1p = tc.alloc_tile_pool(name="b1p", bufs=1, space="PSUM")
    b1s = tc.alloc_tile_pool(name="b1s", bufs=3)
    knn_pass(mm1, idx1_d, "k1", kp_bufs=1)

    b1q = {}
    def b1_gather(t):
        cols = slice(80 * t, 80 * (t + 1))
        flat_t = idx1_d[t].rearrange("p k -> (p k)")
        nc.sync.dma_start(iw1[0:16, cols], flat_t.rearrange("(f p) -> p f", p=16))
        nc.sync.dma_start(iw1[16:32, cols], iw1[0:16, cols])
        nc.sync.dma_start(iw1[32:64, cols], iw1[0:32, cols])
        nc.sync.dma_start(iw1[64:128, cols], iw1[0:64, cols])
        nbr = b1s.tile([P, CB], dt.float32, name="nbr")
        for g in range(2):
            nc.gpsimd.indirect_copy(out=nbr[:, 640 * g:640 * (g + 1)],
                                    data=xstack[:].bitcast(dt.float32),
                                    idxs=iw1[:, 80 * t + 40 * g:80 * t + 40 * (g + 1)],
                                    i_know_ap_gather_is_preferred=True)
        nbxt = b1s.tile([P, CB], dt.float32r, name="nbxt")
        nc.scalar.copy(nbxt[:], nbr[:])
        b1q[t] = nbxt

    b1_gather(0)
    for t in range(NCHUNK):
        if t + 1 < NCHUNK:
            b1_gather(t + 1)
        nbx = b1q.pop(t)[:]
        csl = slice(CB * t, CB * (t + 1))
        for m, rows in enumerate(rows1a):
            msl = slice(128 * m, 128 * m + rows)
            yp = b1p.tile([P, CB], dt.float32, name="yp", space="PSUM")
            for c0, c1 in CSL:
                nc.tensor.matmul(yp[:rows, c0:c1], w1aT_hi[:, msl],
                                 nbx[0:64, c0:c1], start=True, stop=False)
                nc.tensor.matmul(yp[:rows, c0:c1], w1aX[:, msl],
                                 nbx[:, c0:c1], start=False, stop=True)
            uview = u1[m][:, 128 * t:128 * (t + 1), None].to_broadcast(
                [rows, 128, KNN])
            ob = b1s.tile([rows, CB], dt.float32, name="ob1")
            nc.vector.scalar_tensor_tensor(
                out=ob[:].rearrange("p (n k) -> p n k", k=KNN),
                in0=yp[:rows, :].rearrange("p (n k) -> p n k", k=KNN), scalar=1.0,
                in1=uview, op0=ALU.mult, op1=ALU.add,
                accum_out=sum1a[m][:, t:t + 1])
            nc.sync.dma_start(y1a_d[m][:, csl], ob[:])
            tr = b1t.tile([rows, CB], dt.bfloat16, name="tr")
            nc.scalar.activation(tr[:], ob[:], AF.Square,
                                 accum_out=sq1a[m][:, t:t + 1])
    b1s.release()
    b1p.release()
    k2pool = tc.alloc_tile_pool(name="k2pool", bufs=1)

    s1 = [_acc_reduce(nc, per, sum1a[m][:], rows1a[m], f"s1r{m}") for m in range(2)]
    q1 = [_acc_reduce(nc, per, sq1a[m][:], rows1a[m], f"q1r{m}") for m in range(2)]

    # h1 load prefetch issued before the AllReduce so DMA overlaps it
    sumh1 = [ph1.tile([r, NCHUNK], dt.float32, name=f"sh1_{i}") for i, r in enumerate(rows1a)]
    rows1b = (128, 53)
    m1 = [k2pool.tile([r, N], dt.float32, name=f"m1_{i}") for i, r in enumerate(rows1b)]
    sq1b = [ph1.tile([r, NCHUNK], dt.float32, name=f"q1b_{i}") for i, r in enumerate(rows1b)]
    b2pp = tc.alloc_tile_pool(name="b2pp", bufs=2, space="PSUM")
    h1l = tc.alloc_tile_pool(name="h1l", bufs=2)

    h1q = {}
    def h1_load(c):
        csl = slice(CB * c, CB * (c + 1))
        pair = []
        for m, rows in enumerate(rows1a):
            ld = h1l.tile([rows, CB], dt.float32, name=f"h1ld{m}")
            nc.sync.dma_start(ld[:], y1a_d[m][:, csl])
            pair.append(ld)
        h1q[c] = pair

    h1_load(0)
    ar1 = _allreduce(nc, dram, per, [s1[0][:], s1[1][:]], [q1[0][:], q1[1][:]],
                     rows1a, 152, "ar1")
    sb1a = [_bn_coeffs(nc, per, ar1[m][0][:], ar1[m][1][:], g1a[m][:], b1a[m][:],
                       rows1a[m], CNT, f"bn1a{m}") for m in range(2)]
    for c in range(NCHUNK):
        if c + 1 < NCHUNK:
            h1_load(c + 1)
        csl = slice(CB * c, CB * (c + 1))
        h1c = h1q.pop(c)
        for m, rows in enumerate(rows1a):
            ld = h1c[m]
            nc.scalar.activation(ld[:], ld[:], AF.Prelu,
                                 bias=sb1a[m][1][:], scale=sb1a[m][0][:], alpha=0.2,
                                 accum_out=sumh1[m][:, c:c + 1])
        for m, rows in enumerate(rows1b):
            msl = slice(128 * m, 128 * m + rows)
            yp = b2pp.tile([P, CB], dt.float32, name="y1bp", space="PSUM")
            for c0, c1 in CSL:
                nc.tensor.matmul(yp[:rows, c0:c1], wkt1b[0][:, msl],
                                 h1c[0][:, c0:c1], start=True, stop=False)
                nc.tensor.matmul(yp[:rows, c0:c1], wkt1b[1][:, msl],
                                 h1c[1][:, c0:c1], start=False, stop=True)
            tr = b1t.tile([rows, CB], dt.bfloat16, name="tr")
            nc.scalar.activation(tr[:], yp[:rows, :], AF.Square,
                                 accum_out=sq1b[m][:, c:c + 1])
            nc.vector.tensor_reduce(out=m1[m][:, 128 * c:128 * (c + 1)],
                                    in_=yp[:rows, :].rearrange("p (n k) -> p n k", k=KNN),
                                    axis=AX.X, op=ALU.max)
    b2pp.release()
    h1l.release()

    # sum(y1b) = W1b' @ sum(h1)
    sh1 = [_acc_reduce(nc, per, sumh1[m][:], rows1a[m], f"sh1r{m}") for m in range(2)]
    ps1b = tc.alloc_tile_pool(name="ps1b", bufs=1, space="PSUM")
    s1b = []
    for m, rows in enumerate(rows1b):
        msl = slice(128 * m, 128 * m + rows)
        sp = ps1b.tile([rows, 1], dt.float32, name=f"s1bp{m}", space="PSUM")
        nc.tensor.matmul(sp[:], wkt1b[0][:, msl], sh1[0][:], start=True, stop=False)
        nc.tensor.matmul(sp[:], wkt1b[1][:, msl], sh1[1][:], start=False, stop=True)
        st = per.tile([rows, 1], dt.float32, name=f"s1b_{m}")
        nc.scalar.copy(st[:], sp[:])
        s1b.append(st)
    q1b = [_acc_reduce(nc, per, sq1b[m][:], rows1b[m], f"q1br{m}") for m in range(2)]
    ps1b.release()
    ar2 = _allreduce(nc, dram, per, [s1b[0][:], s1b[1][:]], [q1b[0][:], q1b[1][:]],
                     rows1b, 181, "ar2")
    sb1b = [_bn_coeffs(nc, per, ar2[m][0][:], ar2[m][1][:], g1b[m][:], b1b[m][:],
                       rows1b[m], CNT, f"bn1b{m}") for m in range(2)]

    # x1 = prelu(bn(m1)) -> hi/lo splits + sq2, all chunk-wise
    pk2op = tc.alloc_tile_pool(name="pk2op", bufs=1)   # knn2 operands (freed w/ k2pool)
    ones53 = per.tile([53, 1], dt.float32r, name="ones53")
    nc.vector.tensor_copy(ones53[:], ones_f[:53, :])
    ones128 = per.tile([128, 1], dt.float32r, name="ones128")
    nc.vector.tensor_copy(ones128[:], ones_f[:])
    x1hi = [per.tile([r, N], dt.float32r, name=f"x1s{m}_hi") for m, r in enumerate(rows1b)]
    x1lo = [pk2op.tile([r, N], dt.bfloat16, name=f"x1s{m}_lo") for m, r in enumerate(rows1b)]
    r2h_k1 = pk2op.tile([55, N], dt.float32r, name="r2h_k1")
    psq2 = tc.alloc_tile_pool(name="psq2", bufs=1, space="PSUM")
    sqt = tc.alloc_tile_pool(name="sqt", bufs=1)
    sq2p = psq2.tile([1, N], dt.float32, name="sq2p", space="PSUM")
    for c in range(4):
        sl = slice(512 * c, 512 * (c + 1))
        first = True
        for m, rows in enumerate(rows1b):
            x1c = sqt.tile([rows, 512], dt.float32, name="x1c")
            nc.scalar.activation(x1c[:], m1[m][:, sl], AF.Prelu, bias=sb1b[m][1][:],
                                 scale=sb1b[m][0][:], alpha=0.2)
            nc.vector.tensor_copy(x1hi[m][:, sl], x1c[:])
            nc.vector.tensor_tensor(out=x1lo[m][:, sl], in0=x1c[:],
                                    in1=x1hi[m][:, sl].bitcast(dt.float32),
                                    op=ALU.subtract)
            t = sqt.tile([rows, 512], dt.float32, name="x1sqc")
            nc.vector.tensor_tensor(out=t[:], in0=x1c[:], in1=x1c[:], op=ALU.mult)
            h = sqt.tile([rows, 512], dt.float32r, name="x1sqh")
            nc.vector.tensor_copy(h[:], t[:])
            l = sqt.tile([rows, 512], dt.float32r, name="x1sql")
            nc.vector.tensor_tensor(out=l[:], in0=t[:], in1=h[:].bitcast(dt.float32),
                                    op=ALU.subtract)
            on = ones128 if rows == 128 else ones53
            nc.tensor.matmul(sq2p[:, sl], on[:], h[:], start=first, stop=False)
            first = False
            nc.tensor.matmul(sq2p[:, sl], on[:], l[:],
                             start=False, stop=(m == 1))
        sc_ = sqt.tile([1, 512], dt.float32, name="sq2c")
        nc.scalar.copy(sc_[:], sq2p[:, sl])
        sh_ = sqt.tile([1, 512], dt.float32r, name="sq2ch")
        nc.vector.tensor_copy(sh_[:], sc_[:])
        sl_ = sqt.tile([1, 512], dt.float32r, name="sq2cl")
        nc.vector.tensor_tensor(out=sl_[:], in0=sc_[:], in1=sh_[:].bitcast(dt.float32),
                                op=ALU.subtract)
        nh_ = sqt.tile([1, 512], dt.float32r, name="sq2nh")
        nc.vector.tensor_scalar_mul(nh_[:], sh_[:], -1.0)
        nc.sync.dma_start(r2h_k1[53:54, sl], nh_[:])
        nl_ = sqt.tile([1, 512], dt.float32r, name="sq2nl")
        nc.vector.tensor_scalar_mul(nl_[:], sl_[:], -1.0)
        nc.sync.dma_start(r2h_k1[54:55, sl], nl_[:])
    psq2.release()
    sqt.release()

    # knn2 operands: lhsT reuses x1hi/x1lo; only scaled rhs tiles are built
    l2h_k1 = pk2op.tile([55, N], dt.float32r, name="l2h_k1")    # x1hi 128:181 + 2 ones
    nc.vector.tensor_copy(l2h_k1[0:53, :], x1hi[1][:])
    ones2f2 = pk2op.tile([2, N], dt.float32, name="ones2f2")
    nc.vector.memset(ones2f2[:], 1.0)
    ones2n = pk2op.tile([2, N], dt.float32r, name="ones2n")
    nc.vector.tensor_copy(ones2n[:], ones2f2[:])
    nc.sync.dma_start(l2h_k1[53:55, :], ones2n[:])
    r2h_k0 = pk2op.tile([P, N], dt.float32r, name="r2h_k0")     # 2*x1hi[0]
    nc.vector.tensor_scalar_mul(r2h_k0[:], x1hi[0][:], 2.0)
    nc.vector.tensor_scalar_mul(r2h_k1[0:53, :], x1hi[1][:], 2.0)
    r2l0 = pk2op.tile([P, N], dt.bfloat16, name="r2l0")         # 2*x1lo[0]
    nc.vector.tensor_scalar_mul(r2l0[:], x1lo[0][:], 2.0)
    r2l1 = pk2op.tile([53, N], dt.bfloat16, name="r2l1")        # 2*x1lo[1]
    nc.vector.tensor_scalar_mul(r2l1[:], x1lo[1][:], 2.0)
    # bf16 copies of hi operands for the lo-correction matmuls (1 cyc/col)
    hiB0 = pk2op.tile([P, N], dt.bfloat16, name="hiB0")
    nc.vector.tensor_copy(hiB0[:], x1hi[0][:].bitcast(dt.float32))
    hiB1 = pk2op.tile([53, N], dt.bfloat16, name="hiB1")
    nc.vector.tensor_copy(hiB1[:], x1hi[1][:].bitcast(dt.float32))
    r2hB0 = pk2op.tile([P, N], dt.bfloat16, name="r2hB0")
    nc.vector.tensor_copy(r2hB0[:], r2h_k0[:].bitcast(dt.float32))
    r2hB1 = pk2op.tile([53, N], dt.bfloat16, name="r2hB1")
    nc.vector.tensor_copy(r2hB1[:], r2h_k1[0:53, :].bitcast(dt.float32))

    def mm2(pe, t, c):
        msl = slice(128 * t, 128 * (t + 1))
        csl = slice(512 * c, 512 * (c + 1))
        nc.tensor.matmul(pe[:, csl], x1hi[0][:, msl], r2h_k0[:, csl], start=True, stop=False)
        nc.tensor.matmul(pe[:, csl], l2h_k1[:, msl], r2h_k1[:, csl], start=False, stop=False)
        nc.tensor.matmul(pe[:, csl], hiB0[:, msl], r2l0[:, csl], start=False, stop=False)
        nc.tensor.matmul(pe[:, csl], hiB1[:, msl], r2l1[:, csl], start=False, stop=False)
        nc.tensor.matmul(pe[:, csl], x1lo[0][:, msl], r2hB0[:, csl], start=False, stop=False)
        nc.tensor.matmul(pe[:, csl], x1lo[1][:, msl], r2hB1[:, csl], start=False, stop=True)

    iw2 = per.tile([P, S // 16], dt.uint16, name="iw_k2")
    iw2d = per.tile([P, S // 16], dt.uint16, name="iw2d")
    knn_pass(mm2, idx2_d, "k2")
    pk2op.release()
    k2pool.release()
    ph1.release()

    # ---------------- block 2 ----------------
    # gather pools first: they reuse ph1/k2pool bytes whose users finished
    # before knn2, so the b2a gather stream can overlap knn2
    b2g = tc.alloc_tile_pool(name="b2g", bufs=1)
    gxi = b2g.tile([P, 2 * N], dt.bfloat16, name="gxi")
    _g3 = gxi[:].rearrange("p (n two) -> p n two", two=2)
    nc.scalar.copy(_g3[:, :, 0], x1hi[0][:].bitcast(dt.float32))
    nc.scalar.copy(_g3[:, :, 1], x1hi[0][:].bitcast(dt.float32))
    nc.scalar.copy(_g3[0:53, :, 1], x1hi[1][:].bitcast(dt.float32))
    b2s = tc.alloc_tile_pool(name="b2s", bufs=4)

    # conv2b weights early so their DMA overlaps b2a / AR3
    b2bpool = tc.alloc_tile_pool(name="b2bpool", bufs=1)
    w2bT_r, w2bT_b = [], []
    for i, rows in enumerate((128, 128, 128, 46)):
        wt = wld.tile([128, 595], dt.float32, name="wf")
        nc.sync.dma_start(wt[:rows, :512], ins[f"w2bT_k{i}"])
        wr = b2bpool.tile([rows, 512], dt.float32r, name=f"w2bT_r{i}")
        nc.vector.tensor_copy(wr[:], wt[:rows, :512])
        w2bT_r.append(wr)
        wb = b2bpool.tile([rows, 512], dt.bfloat16, name=f"w2bT_b{i}")
        nc.vector.tensor_copy(wb[:], wt[:rows, :512])
        w2bT_b.append(wb)

    # U2[c, n] = sum_ci du2[ci, c] x1[ci, n]   (plain f32r)
    b2pool = tc.alloc_tile_pool(name="b2pool", bufs=1)
    w2aT_r, du2T_r = [], []
    for i, rows in enumerate((128, 53)):
        wt = wld.tile([128, 595], dt.float32, name="wf")
        nc.sync.dma_start(wt[:rows, :430], ins[f"w2aT_k{i}"])
        wr = b2pool.tile([rows, 430], dt.float32r, name=f"w2aT_r{i}")
        nc.vector.tensor_copy(wr[:], wt[:rows, :430])
        w2aT_r.append(wr)
        ddt = wld.tile([128, 595], dt.float32, name="wf")
        nc.sync.dma_start(ddt[:rows, :430], ins[f"du2T_k{i}"])
        dr = b2pool.tile([rows, 430], dt.float32r, name=f"du2T_r{i}")
        nc.vector.tensor_copy(dr[:], ddt[:rows, :430])
        du2T_r.append(dr)
    rows2a_ = (128, 128, 128, 46)
    u2 = [b2pool.tile([r, N], dt.float32, name=f"u2_{i}") for i, r in enumerate(rows2a_)]
    pu2 = tc.alloc_tile_pool(name="pu2", bufs=4, space="PSUM")
    for m, rows in enumerate(rows2a_):
        msl = slice(128 * m, 128 * m + rows)
        for cc in range(4):
            sl = slice(512 * cc, 512 * (cc + 1))
            up = pu2.tile([rows, 512], dt.float32, name="up2", space="PSUM")
            nc.tensor.matmul(up[:], du2T_r[0][:, msl], x1hi[0][:, sl], start=True, stop=False)
            nc.tensor.matmul(up[:], du2T_r[1][:, msl], x1hi[1][:, sl], start=False, stop=True)
            nc.scalar.copy(u2[m][:, sl], up[:])
    pu2.release()

    rows2a = (128, 128, 128, 46)
    y2a_d = [dram.tile([r, S], dt.bfloat16, name=f"y2a_d{i}") for i, r in enumerate(rows2a)]
    sum2a = [b2pool.tile([r, NCHUNK], dt.float32, name=f"s2a_{i}") for i, r in enumerate(rows2a)]
    sq2a = [b2pool.tile([r, NCHUNK], dt.float32, name=f"q2a_{i}") for i, r in enumerate(rows2a)]

    w2aT_b = []
    for i, rows in enumerate((128, 53)):
        wb = b2pool.tile([rows, 430], dt.bfloat16, name=f"w2aT_b{i}")
        nc.vector.tensor_copy(wb[:], w2aT_r[i][:].bitcast(dt.float32))
        w2aT_b.append(wb)

    b2p = tc.alloc_tile_pool(name="b2p", bufs=2, space="PSUM")
    b2o = tc.alloc_tile_pool(name="b2o", bufs=3)
    gxi3 = gxi[:].rearrange("p (n two) -> p n two", two=2)
    b2q = {}
    def b2_gather(c):
        cols = slice(80 * c, 80 * (c + 1))
        flat_t = idx2_d[c].rearrange("p k -> (p k)")
        nc.sync.dma_start(iw2[0:16, cols], flat_t.rearrange("(f p) -> p f", p=16))
        nc.sync.dma_start(iw2[16:32, cols], iw2[0:16, cols])
        nc.sync.dma_start(iw2[32:64, cols], iw2[0:32, cols])
        nc.sync.dma_start(iw2[64:128, cols], iw2[0:64, cols])
        nc.scalar.mul(iw2d[:, cols], iw2[:, cols], 2.0)
        nbi = b2s.tile([P, 2 * CB], dt.bfloat16, name="nbi")
        nbi3 = nbi[:].rearrange("p (n two) -> p n two", two=2)
        for e0, e1 in ((0, 512), (512, 1024), (1024, 1280)):
            nc.gpsimd.indirect_copy(
                out=nbi3[:, e0:e1, :], data=gxi3,
                idxs=iw2d[:, 80 * c + e0 // 16:80 * c + e1 // 16],
                i_know_ap_gather_is_preferred=True)
        nb0r = b2s.tile([P, CB], dt.bfloat16, name="nb0r")
        nc.vector.tensor_copy(nb0r[:], nbi3[:, :, 0])
        nb1r = b2s.tile([P, CB], dt.bfloat16, name="nb1r")
        nc.vector.tensor_copy(nb1r[:], nbi3[:, :, 1])
        b2q[c] = (nb0r, nb1r)

    for _pc in range(4):
        b2_gather(_pc)
    for c in range(NCHUNK):
        if c + 4 < NCHUNK:
            b2_gather(c + 4)
        nb0r, nb1r = b2q.pop(c)
        for m, rows in enumerate(rows2a):
            msl = slice(128 * m, 128 * m + rows)
            yp = b2p.tile([P, CB], dt.float32, name="y2ap", space="PSUM")
            for c0, c1 in CSL:
                nc.tensor.matmul(yp[:rows, c0:c1], w2aT_b[0][:, msl],
                                 nb0r[:, c0:c1], start=True, stop=False)
                nc.tensor.matmul(yp[:rows, c0:c1], w2aT_b[1][:, msl],
                                 nb1r[0:53, c0:c1], start=False, stop=True)
            uview = u2[m][:, 128 * c:128 * (c + 1), None].to_broadcast(
                [rows, 128, KNN])
            ob = b2o.tile([rows, CB], dt.bfloat16, name="ob2")
            nc.vector.scalar_tensor_tensor(
                out=ob[:].rearrange("p (n k) -> p n k", k=KNN),
                in0=yp[:rows, :].rearrange("p (n k) -> p n k", k=KNN), scalar=1.0,
                in1=uview, op0=ALU.mult, op1=ALU.add,
                accum_out=sum2a[m][:, c:c + 1])
            nc.sync.dma_start(y2a_d[m][:, CB * c:CB * (c + 1)], ob[:])
            tr = b1t.tile([rows, CB], dt.bfloat16, name="tr")
            nc.scalar.activation(tr[:], ob[:], AF.Square,
                                 accum_out=sq2a[m][:, c:c + 1])
    b2p.release()
    b2o.release()

    s2 = [_acc_reduce(nc, per, sum2a[m][:], rows2a[m], f"s2r{m}") for m in range(4)]
    q2 = [_acc_reduce(nc, per, sq2a[m][:], rows2a[m], f"q2r{m}") for m in range(4)]
    b2pool.release()

    # block-3 weights early (LIFO: pw3 below b3p/b3l/c3t/px2r); DMA overlaps conv2b
    pw3 = tc.alloc_tile_pool(name="pw3", bufs=1)
    w3aT_r = []
    rows3a_k = (128, 53, 128, 128, 128, 128)
    for i, rows in enumerate(rows3a_k):
        wt = wld.tile([128, 595], dt.float32, name="wf")
        nc.sync.dma_start(wt[:rows, :595], ins[f"w3aT_k{i}"])
        wr = pw3.tile([rows, 595], dt.bfloat16, name=f"w3aT_r{i}")
        nc.vector.tensor_copy(wr[:], wt[:rows, :595])
        w3aT_r.append(wr)
    w3bT_r = []
    rows3b_k = (128, 128, 128, 128, 83)
    for i, rows in enumerate(rows3b_k):
        wt = wld.tile([128, 595], dt.float32, name="wf")
        nc.sync.dma_start(wt[:rows, :512], ins[f"w3bT_k{i}"])
        wr = pw3.tile([rows, 512], dt.bfloat16, name=f"w3bT_r{i}")
        nc.vector.tensor_copy(wr[:], wt[:rows, :512])
        w3bT_r.append(wr)
    pb3 = tc.alloc_tile_pool(name="pb3", bufs=1)   # y3a/y3b bf16 residency

    rows2b = (128, 128, 128, 128)
    sumh2 = [b2bpool.tile([r, NCHUNK], dt.float32, name=f"sh2_{i}") for i, r in enumerate(rows2a)]
    sq2b = [b2bpool.tile([r, NCHUNK], dt.float32, name=f"q2b_{i}") for i, r in enumerate(rows2b)]
    m2 = [b2bpool.tile([r, N], dt.bfloat16, name=f"m2_{i}") for i, r in enumerate(rows2b)]

    b3p = tc.alloc_tile_pool(name="b3p", bufs=2, space="PSUM")
    b3l = tc.alloc_tile_pool(name="b3l", bufs=2)

    c2q = {}
    def c2_load(c):
        csl = slice(CB * c, CB * (c + 1))
        grp = []
        for m, rows in enumerate(rows2a):
            ld = b3l.tile([rows, CB], dt.bfloat16, name=f"ld{m}")
            nc.sync.dma_start(ld[:], y2a_d[m][:, csl])
            grp.append(ld)
        c2q[c] = grp

    c2_load(0)
    ar3 = _allreduce(nc, dram, per, [s[:] for s in s2], [q[:] for q in q2],
                     rows2a, 430, "ar3")
    sb2a = [_bn_coeffs(nc, per, ar3[m][0][:], ar3[m][1][:], g2a[m][:], b2a[m][:],
                       rows2a[m], CNT, f"bn2a{m}") for m in range(4)]
    for c in range(NCHUNK):
        if c + 1 < NCHUNK:
            c2_load(c + 1)
        csl = slice(CB * c, CB * (c + 1))
        h2 = c2q.pop(c)
        for m, rows in enumerate(rows2a):
            ld = h2[m]
            nc.scalar.activation(ld[:], ld[:], AF.Prelu, bias=sb2a[m][1][:],
                                 scale=sb2a[m][0][:], alpha=0.2,
                                 accum_out=sumh2[m][:, c:c + 1])
        for m, rows in enumerate(rows2b):
            msl = slice(128 * m, 128 * (m + 1))
            yp = b3p.tile([P, CB], dt.float32, name="y2bp", space="PSUM")
            for c0, c1 in CSL:
                for k in range(4):
                    kr = (128, 128, 128, 46)[k]
                    nc.tensor.matmul(yp[:rows, c0:c1], w2bT_b[k][:, msl],
                                     h2[k][0:kr, c0:c1],
                                     start=(k == 0), stop=(k == 3))
            tr = b1t.tile([rows, CB], dt.bfloat16, name="tr")
            nc.scalar.activation(tr[:], yp[:rows, :], AF.Square,
                                 accum_out=sq2b[m][:, c:c + 1])
            nc.vector.tensor_reduce(out=m2[m][:, 128 * c:128 * (c + 1)],
                                    in_=yp[:rows, :].rearrange("p (n k) -> p n k", k=KNN),
                                    axis=AX.X, op=ALU.max)
    b3p.release()
    b3l.release()

    sh2 = [_acc_reduce(nc, per, sumh2[m][:], rows2a[m], f"sh2r{m}") for m in range(4)]
    sh2r = []
    for m in range(4):
        t = per.tile([rows2a[m], 4], dt.float32r, name=f"sh2rr{m}")
        nc.vector.tensor_copy(t[:], sh2[m][:].to_broadcast([rows2a[m], 4]))
        sh2r.append(t)
    ps2b = tc.alloc_tile_pool(name="ps2b", bufs=1, space="PSUM")
    s2b = []
    for m in range(4):
        msl = slice(128 * m, 128 * (m + 1))
        sp = ps2b.tile([128, 4], dt.float32, name=f"s2bp{m}", space="PSUM")
        for k in range(4):
            kr = (128, 128, 128, 46)[k]
            nc.tensor.matmul(sp[:], w2bT_r[k][:, msl], sh2r[k][0:kr, :],
                             start=(k == 0), stop=(k == 3))
        st = per.tile([128, 1], dt.float32, name=f"s2b_{m}")
        nc.scalar.copy(st[:], sp[:, 0:1])
        s2b.append(st)
    ps2b.release()
    q2b = [_acc_reduce(nc, per, sq2b[m][:], 128, f"q2br{m}") for m in range(4)]
    ar4 = _allreduce(nc, dram, per, [s[:] for s in s2b], [q[:] for q in q2b],
                     rows2b, 512, "ar4")
    sb2b = [_bn_coeffs(nc, per, ar4[m][0][:], ar4[m][1][:], g2b[m][:], b2b[m][:],
                       128, CNT, f"bn2b{m}") for m in range(4)]

    # x2 = prelu(bn(m2)) -> bf16; x1 recast to bf16 for block 3
    c3t = tc.alloc_tile_pool(name="c3t", bufs=2)
    px2r = tc.alloc_tile_pool(name="px2r", bufs=1)
    x2r = [px2r.tile([128, N], dt.bfloat16, name=f"x2r_{i}") for i in range(4)]
    for m in range(4):
        nc.scalar.activation(x2r[m][:], m2[m][:], AF.Prelu, bias=sb2b[m][1][:],
                             scale=sb2b[m][0][:], alpha=0.2)
    xb1 = [px2r.tile([r, N], dt.bfloat16, name=f"xb1_{i}")
           for i, r in enumerate((128, 53))]
    for i in range(2):
        nc.vector.tensor_copy(xb1[i][:], x1hi[i][:].bitcast(dt.float32))

    # ---------------- block 3 ----------------
    # y3a / y3b stay SBUF-resident in bf16 (post-x1 path tolerates bf16)
    xc_k = [xb1[0], xb1[1], x2r[0], x2r[1], x2r[2], x2r[3]]
    rows3a = (128, 128, 128, 128, 83)
    y3a_s = [pb3.tile([r, N], dt.bfloat16, name=f"y3a_s{i}") for i, r in enumerate(rows3a)]
    sum3a = [per.tile([r, 4], dt.float32, name=f"s3a_{i}") for i, r in enumerate(rows3a)]
    sq3a = [per.tile([r, 4], dt.float32, name=f"q3a_{i}") for i, r in enumerate(rows3a)]
    c3p = tc.alloc_tile_pool(name="c3p", bufs=4, space="PSUM")
    for c in range(4):
        csl = slice(512 * c, 512 * (c + 1))
        for m, rows in enumerate(rows3a):
            msl = slice(128 * m, 128 * m + rows)
            yp = c3p.tile([128, 512], dt.float32, name="y3ap", space="PSUM")
            for k in range(6):
                nc.tensor.matmul(yp[:rows, :], w3aT_r[k][:, msl], xc_k[k][:, csl],
                                 start=(k == 0), stop=(k == 5))
            nc.scalar.activation(y3a_s[m][:, csl], yp[:rows, :], AF.Copy,
                                 accum_out=sum3a[m][:, c:c + 1])
            tr = c3t.tile([rows, 512], dt.bfloat16, name="tr")
            nc.scalar.activation(tr[:], yp[:rows, :], AF.Square,
                                 accum_out=sq3a[m][:, c:c + 1])
    c3p.release()
    px2r.release()

    s3 = [_acc_reduce(nc, per, sum3a[m][:], rows3a[m], f"s3r{m}") for m in range(5)]
    q3 = [_acc_reduce(nc, per, sq3a[m][:], rows3a[m], f"q3r{m}") for m in range(5)]
    ar5 = _allreduce(nc, dram, per, [s[:] for s in s3], [q[:] for q in q3],
                     rows3a, 640, "ar5")
    sb3a = [_bn_coeffs(nc, per, ar5[m][0][:], ar5[m][1][:], g3a[m][:], b3a[m][:],
                       rows3a[m], CNT3, f"bn3a{m}") for m in range(5)]

    rows3b = (128, 128, 128, 128)
    y3b_s = [pb3.tile([r, N], dt.bfloat16, name=f"y3b_s{i}") for i, r in enumerate(rows3b)]
    sum3b = [per.tile([r, 4], dt.float32, name=f"s3b_{i}") for i, r in enumerate(rows3b)]
    sq3b = [per.tile([r, 4], dt.float32, name=f"q3b_{i}") for i, r in enumerate(rows3b)]
    c4p = tc.alloc_tile_pool(name="c4p", bufs=2, space="PSUM")
    h3l = tc.alloc_tile_pool(name="h3l", bufs=6)
    for c in range(4):
        csl = slice(512 * c, 512 * (c + 1))
        h3c = []
        for k, kr in enumerate(rows3a):
            ld = h3l.tile([kr, 512], dt.bfloat16, name="h3ld")
            nc.scalar.activation(ld[:], y3a_s[k][:, csl], AF.Prelu,
                                 bias=sb3a[k][1][:], scale=sb3a[k][0][:], alpha=0.2)
            h3c.append(ld)
        for m, rows in enumerate(rows3b):
            msl = slice(128 * m, 128 * (m + 1))
            yp = c4p.tile([rows, 512], dt.float32, name=f"y3bp{m}", space="PSUM")
            for k in range(5):
                nc.tensor.matmul(yp[:], w3bT_r[k][:, msl], h3c[k][:],
                                 start=(k == 0), stop=(k == 4))
            nc.scalar.activation(y3b_s[m][:, csl], yp[:], AF.Copy,
                                 accum_out=sum3b[m][:, c:c + 1])
            tr = c3t.tile([rows, 512], dt.bfloat16, name="tr")
            nc.scalar.activation(tr[:], yp[:], AF.Square,
                                 accum_out=sq3b[m][:, c:c + 1])
    c4p.release()
    h3l.release()
    c3t.release()

    s4 = [_acc_reduce(nc, per, sum3b[m][:], 128, f"s4r{m}") for m in range(4)]
    q4 = [_acc_reduce(nc, per, sq3b[m][:], 128, f"q4r{m}") for m in range(4)]
    ar6 = _allreduce(nc, dram, per, [s[:] for s in s4], [q[:] for q in q4],
                     rows3b, 512, "ar6")
    sb3b = [_bn_coeffs(nc, per, ar6[m][0][:], ar6[m][1][:], g3b[m][:], b3b[m][:],
                       128, CNT3, f"bn3b{m}") for m in range(4)]

    out_d = nc._ext_outputs["out"]
    for m in range(4):
        for c in range(4):
            csl = slice(512 * c, 512 * (c + 1))
            ot = oseg.tile([128, 512], dt.float32, name="ot")
            nc.scalar.activation(ot[:], y3b_s[m][:, csl], AF.Prelu,
                                 bias=sb3b[m][1][:], scale=sb3b[m][0][:], alpha=0.2)
            nc.sync.dma_start(out_d[128 * m:128 * (m + 1), csl], ot[:])
    pb3.release()
    pw3.release()
    b2bpool.release()
    b2s.release()
    b2g.release()
    wld.release()
    oseg.release()
    b1t.release()
    per.release()


# ------------------------------------------------------------------ host side
_CACHE = {}


def _build():
    _install_bassfix()
    nc = bass.Bass("TRN2", target_bir_lowering=False, debug=False,
                   num_devices=NCORES)
    in_specs = {
        "x": (64, N), "w1aT": (64, 152), "du1T": (64, 152),
        "w1bT_k0": (128, 181), "w1bT_k1": (24, 181),
        "w2aT_k0": (128, 430), "w2aT_k1": (53, 430),
        "du2T_k0": (128, 430), "du2T_k1": (53, 430),
        "w2bT_k0": (128, 512), "w2bT_k1": (128, 512),
        "w2bT_k2": (128, 512), "w2bT_k3": (46, 512),
        "w3aT_k0": (128, 595), "w3aT_k1": (53, 595), "w3aT_k2": (128, 595),
        "w3aT_k3": (128, 595), "w3aT_k4": (128, 595), "w3aT_k5": (128, 595),
        "w3bT_k0": (128, 512), "w3bT_k1": (128, 512), "w3bT_k2": (128, 512),
        "w3bT_k3": (128, 512), "w3bT_k4": (83, 512),
        "g1a": (152, 1), "be1a": (152, 1), "g1b": (181, 1), "be1b": (181, 1),
        "g2a": (430, 1), "be2a": (430, 1), "g2b": (512, 1), "be2b": (512, 1),
        "g3a": (595, 1), "be3a": (595, 1), "g3b": (512, 1), "be3b": (512, 1),
        "E": (32, CB),
    }
    nc._ext_inputs = {}
    for nm, shp in in_specs.items():
        nc._ext_inputs[nm] = nc.dram_tensor(nm, list(shp), dt.float32,
                                            kind="ExternalInput").ap()
    nc._ext_outputs = {
        "out": nc.dram_tensor("out", [512, N], dt.float32,
                              kind="ExternalOutput").ap()}
    with tile.TileContext(nc) as tc:
        build_kernel(nc, tc)
    return nc


def _host_inputs(x, c1w1, c1g1, c1be1, c1w2, c1g2, c1be2,
                 c2w1, c2g1, c2be1, c2w2, c2g2, c2be2,
                 c3w1, c3g1, c3be1, c3w2, c3g2, c3be2):
    f32 = np.float32
    W1 = np.asarray(c1w1, f32)            # [152, 128]
    W1a, W1b = W1[:, :64], W1[:, 64:]
    W2 = np.asarray(c2w1, f32)            # [430, 362]
    W2a, W2b = W2[:, :181], W2[:, 181:]
    ws = {
        "w1aT": np.ascontiguousarray(W1a.T),
        "du1T": np.ascontiguousarray((W1b - W1a).T),
    }
    w1bT = np.ascontiguousarray(np.asarray(c1w2, f32).T)     # [152, 181]
    ws["w1bT_k0"], ws["w1bT_k1"] = w1bT[:128], w1bT[128:]
    w2aT = np.ascontiguousarray(W2a.T)                        # [181, 430]
    du2T = np.ascontiguousarray((W2b - W2a).T)
    ws["w2aT_k0"], ws["w2aT_k1"] = w2aT[:128], w2aT[128:]
    ws["du2T_k0"], ws["du2T_k1"] = du2T[:128], du2T[128:]
    w2bT = np.ascontiguousarray(np.asarray(c2w2, f32).T)     # [430, 512]
    for i, sl in enumerate((slice(0, 128), slice(128, 256), slice(256, 384),
                            slice(384, 430))):
        ws[f"w2bT_k{i}"] = w2bT[sl]
    w3aT = np.ascontiguousarray(np.asarray(c3w1, f32).T)     # [693, 595]
    cuts = (0, 128, 181, 309, 437, 565, 693)
    for i in range(6):
        ws[f"w3aT_k{i}"] = w3aT[cuts[i]:cuts[i + 1]]
    w3bT = np.ascontiguousarray(np.asarray(c3w2, f32).T)     # [595, 512]
    for i in range(5):
        ws[f"w3bT_k{i}"] = w3bT[128 * i:min(128 * (i + 1), 595)]
    for nm, v in (("g1a", c1g1), ("be1a", c1be1), ("g1b", c1g2), ("be1b", c1be2),
                  ("g2a", c2g1), ("be2a", c2be1), ("g2b", c2g2), ("be2b", c2be2),
                  ("g3a", c3g1), ("be3a", c3be1), ("g3b", c3g2), ("be3b", c3be2)):
        ws[nm] = np.ascontiguousarray(np.asarray(v, f32).reshape(-1, 1))
    E = np.zeros((32, CB), f32)
    for r in range(32):
        E[r, KNN * r:KNN * (r + 1)] = 1.0
    ws["E"] = E
    ws = {k: np.ascontiguousarray(v, f32) for k, v in ws.items()}
    xs = np.asarray(x, f32)
    in_maps = []
    for i in range(NCORES):
        m = dict(ws)
        m["x"] = np.ascontiguousarray(xs[i])
        in_maps.append(m)
    return in_maps


def kernel(x, c1w1, c1b1, c1g1, c1be1, c1w2, c1b2, c1g2, c1be2,
           c2w1, c2b1, c2g1, c2be1, c2w2, c2b2, c2g2, c2be2,
           c3w1, c3b1, c3g1, c3be1, c3w2, c3b2, c3g2, c3be2):
    # conv biases are absorbed exactly by the following BatchNorm (shift
    # invariance), so b* inputs are unused by the device program.
    if "nc" not in _CACHE:
        _CACHE["nc"] = _build()
    nc = _CACHE["nc"]
    in_maps = _host_inputs(x, c1w1, c1g1, c1be1, c1w2, c1g2, c1be2,
                           c2w1, c2g1, c2be1, c2w2, c2g2, c2be2,
                           c3w1, c3g1, c3be1, c3w2, c3g2, c3be2)
    res = run_bass_kernel_spmd(nc, in_maps, core_ids=list(range(NCORES)))
    _CACHE["last_result"] = res
    out = np.stack([res.results[i]["out"] for i in range(NCORES)], axis=0)
    return out.astype(np.float32)



# revision 44
# speedup vs baseline: 1.1880x; 1.0219x over previous
"""DGCNN-style GCN encoder on 8 TRN2 NeuronCores (bass/tile).

Data-parallel over batch: each core owns one sample (B=8). BatchNorm batch
statistics are all-reduced across the 8 cores so results match global-batch
BN exactly. kNN top-10, neighbor gathers, and all convs/BN/activations run
on-device; the host only reshapes/replicates weights.

Precision strategy: matmuls on the x -> x1 -> knn2 path use a compensated
fp32r (hi/lo split) scheme that recovers ~fp32 accuracy at fp32r speed
(conv1b uses plain fp32). This keeps the kNN neighbor selection stable
against the f32 reference. Everything downstream of x1 (conv2a/2b, block 3)
runs in bf16 (weights + activations), which the 2e-2 tolerance absorbs.

Performance notes (vs the first working version):
- edge loops process 1280-edge chunks (128 points x 10 nbrs) instead of 320
- neighbor gathers are indirect_copy, whose cost is ~28ns per index of Q7
  dispatch; conv2a gathers both x1 row-tiles in ONE pass by interleaving
  them as bf16 pairs (d=2 gather with doubled indices), halving index count
- gathers/loads are software-pipelined one-to-two chunks ahead, and the
  next phase's first loads + weight loads are issued before each BN
  AllReduce so DMA overlaps the collective
- block-1's gather stream overlaps knn1 (per-tile iw builds + pools
  pre-allocated before the knn pass so gather t only waits on top-k tile t);
  iw2/iw2d are likewise built per-tile during knn2
- knn2's lo-correction matmuls (4 of 6) run in bf16 at 1 cyc/col; only the
  two hi*hi + sq-fold matmuls stay compensated f32r, keeping neighbor
  selection error ~5e-5 absolute
- y3a/y3b stay SBUF-resident in bf16 (no DRAM round-trip); y2a is spilled
  to DRAM in bf16 (half the traffic)
"""

import os
import sys
import json

B16 = "/nix/store/wxap7svlj45h0lfm31d1axjjnzyl6qsy-b16-bazel-unstable-cc-2026-05-04-9a3fa1f3-rt-2026-05-04-ade39e0a/lib/python3.13/site-packages"
if B16 not in sys.path:
    sys.path.insert(0, B16)
if "/opt/trn_rl_repo" not in sys.path:
    sys.path.insert(0, "/opt/trn_rl_repo")

import numpy as np
import concourse.bass as bass
import concourse.mybir as mybir
import concourse.tile as tile
from concourse.bass_utils import run_bass_kernel_spmd

dt = mybir.dt
AF = mybir.ActivationFunctionType
ALU = mybir.AluOpType
AX = mybir.AxisListType

N = 2048
KNN = 10
S = N * KNN            # 20480 edge positions
CB = 1280              # banded chunk: 128 points x 10 neighbors
NCHUNK = S // CB       # 16
CSL = [(0, 512), (512, 1024), (1024, 1280)]   # matmul col slices of a chunk
EPS = 1e-5
NCORES = 8
CNT = float(NCORES * S)      # BN count for edge tensors
CNT3 = float(NCORES * N)     # BN count for block-3 tensors


# ---------------------------------------------------------------- walrus fix
def _fix_bir_json(bir_json: bytes) -> bytes:
    """This container's walrus accepts only ONE sync-wait per instruction;
    hoist extra waits onto preceding EventSemaphore instructions."""
    m = json.loads(bir_json)
    for f in m["functions"]:
        for bb in f["blocks"]:
            out = []
            for i in bb["instructions"]:
                si = i.get("sync_info") or {}
                waits = si.get("on_wait") or []
                if len(waits) > 1:
                    for k, w in enumerate(waits[:-1]):
                        out.append({
                            "name": f"{i['name']}w{k}",
                            "engine": i["engine"],
                            "opcode": "EventSemaphore",
                            "ins": [], "outs": [],
                            "debug": i.get("debug"),
                            "sync_info": {"on_update": [], "on_wait": [w]},
                        })
                    i = dict(i)
                    si = dict(si)
                    si["on_wait"] = [waits[-1]]
                    i["sync_info"] = si
                out.append(i)
            bb["instructions"] = out
    return json.dumps(m).encode()


def _install_bassfix():
    import concourse.bass_utils as bu
    import concourse.bass2jax as b2j
    if getattr(bu, "_bassfix_installed", False):
        return
    real = bu.compile_bir_kernel

    def patched(bir_json, tmpdir, neff_name="file.neff"):
        return real(_fix_bir_json(bir_json), tmpdir, neff_name)

    bu.compile_bir_kernel = patched
    b2j.compile_bir_kernel = patched
    bu._bassfix_installed = True


# ------------------------------------------------------------- device helpers
def _round_split(nc, pool, src_ap, parts, width, name):
    """Return (hi, lo) float32r tiles for a f32 source AP [parts, width]."""
    hi = pool.tile([parts, width], dt.float32r, name=name + "_hi")
    lo = pool.tile([parts, width], dt.float32r, name=name + "_lo")
    nc.vector.tensor_copy(hi[:], src_ap)
    nc.vector.tensor_tensor(out=lo[:], in0=src_ap, in1=hi[:].bitcast(dt.float32),
                            op=ALU.subtract)
    return hi, lo


def _bn_coeffs(nc, pool, sums, sqs, g_ap, b_ap, rows, cnt, name):
    """sums/sqs: [rows,1] f32 (global). Returns (scale, bias) [rows,1] f32."""
    mean = pool.tile([rows, 1], dt.float32, name=name + "_mean")
    nc.vector.tensor_scalar_mul(mean[:], sums, 1.0 / cnt)
    e2 = pool.tile([rows, 1], dt.float32, name=name + "_e2")
    nc.vector.tensor_scalar_mul(e2[:], sqs, 1.0 / cnt)
    m2 = pool.tile([rows, 1], dt.float32, name=name + "_m2")
    nc.vector.tensor_tensor(out=m2[:], in0=mean[:], in1=mean[:], op=ALU.mult)
    var = pool.tile([rows, 1], dt.float32, name=name + "_var")
    nc.vector.tensor_tensor(out=var[:], in0=e2[:], in1=m2[:], op=ALU.subtract)
    epst = pool.tile([rows, 1], dt.float32, name=name + "_eps")
    nc.vector.memset(epst[:], float(EPS))
    std = pool.tile([rows, 1], dt.float32, name=name + "_std")
    nc.scalar.activation(std[:], var[:], AF.Sqrt, bias=epst[:])
    rstd = pool.tile([rows, 1], dt.float32, name=name + "_rstd")
    nc.vector.reciprocal(rstd[:], std[:])
    scale = pool.tile([rows, 1], dt.float32, name=name + "_scale")
    nc.vector.tensor_tensor(out=scale[:], in0=rstd[:], in1=g_ap, op=ALU.mult)
    ms = pool.tile([rows, 1], dt.float32, name=name + "_ms")
    nc.vector.tensor_tensor(out=ms[:], in0=mean[:], in1=scale[:], op=ALU.mult)
    bias = pool.tile([rows, 1], dt.float32, name=name + "_bias")
    nc.vector.tensor_tensor(out=bias[:], in0=b_ap, in1=ms[:], op=ALU.subtract)
    return scale, bias


def _allreduce(nc, dram, per, sums_list, sqs_list, rows_list, cpad, name):
    """Pack per-Mtile [rows,1] sum/sq into a DRAM [cpad,2] buffer, AllReduce
    across the 8 cores, unpack back into fresh [rows,1] tiles."""
    cin = dram.tile([cpad, 2], dt.float32, name=name + "_in")
    cout = dram.tile([cpad, 2], dt.float32, name=name + "_out")
    r0 = 0
    for s_, q_, rows in zip(sums_list, sqs_list, rows_list):
        nc.sync.dma_start(cin[r0:r0 + rows, 0:1], s_)
        nc.sync.dma_start(cin[r0:r0 + rows, 1:2], q_)
        r0 += rows
    if r0 < cpad:
        z = per.tile([cpad - r0, 2], dt.float32, name=name + "_z")
        nc.vector.memset(z[:], 0.0)
        nc.sync.dma_start(cin[r0:cpad, :], z[:])
    nc.gpsimd.collective_compute(
        "AllReduce", ALU.add, replica_groups=[list(range(NCORES))],
        ins=[cin[:].opt()], outs=[cout[:].opt()])
    outs = []
    r0 = 0
    for rows in rows_list:
        gs = per.tile([rows, 1], dt.float32, name=name + f"_gs{r0}")
        gq = per.tile([rows, 1], dt.float32, name=name + f"_gq{r0}")
        nc.sync.dma_start(gs[:], cout[r0:r0 + rows, 0:1])
        nc.sync.dma_start(gq[:], cout[r0:r0 + rows, 1:2])
        outs.append((gs, gq))
        r0 += rows
    return outs


def _acc_reduce(nc, per, acc, rows, name):
    """[rows, NCHUNK] accumulator -> [rows,1]."""
    out = per.tile([rows, 1], dt.float32, name=name)
    nc.vector.tensor_reduce(out=out[:], in_=acc, axis=AX.X, op=ALU.add)
    return out


def build_kernel(nc, tc):
    P = 128
    per = tc.alloc_tile_pool(name="persist", bufs=1)
    dram = tc.alloc_tile_pool(name="dram", bufs=1, space="DRAM")
    b1t = tc.alloc_tile_pool(name="b1t", bufs=2)
    oseg = tc.alloc_tile_pool(name="oseg", bufs=1)
    wld = tc.alloc_tile_pool(name="wld", bufs=2)
    ph1 = tc.alloc_tile_pool(name="ph1", bufs=1)

    ins = nc._ext_inputs
    x_d = ins["x"]

    # ---------------- load & prep ----------------
    tmp0 = tc.alloc_tile_pool(name="tmp0", bufs=1)      # until U1t/rhs built
    x = tmp0.tile([64, N], dt.float32, name="x")
    nc.sync.dma_start(x[:], x_d)
    xhi, xlo = _round_split(nc, tmp0, x[:], 64, N, "x")
    # stacks for compensated matmuls
    xstack = ph1.tile([P, N], dt.float32r, name="xstack")   # [xhi; xlo]
    nc.vector.tensor_copy(xstack[:64, :], xhi[:])
    nc.vector.tensor_copy(xstack[64:, :], xlo[:])
    lhsx = xstack                                             # [xhi; xlo]
    rhsx = ph1.tile([P, N], dt.float32r, name="rhsx")        # 2*[xlo; xhi]
    nc.vector.tensor_scalar_mul(rhsx[:64, :], xlo[:], 2.0)
    nc.vector.tensor_scalar_mul(rhsx[64:, :], xhi[:], 2.0)

    # sq1 = sum_c x^2 (compensated)
    xsq = tmp0.tile([64, N], dt.float32, name="xsq")
    nc.vector.tensor_tensor(out=xsq[:], in0=x[:], in1=x[:], op=ALU.mult)
    xsqhi, xsqlo = _round_split(nc, tmp0, xsq[:], 64, N, "xsq")
    ones_f = per.tile([128, 1], dt.float32, name="ones_f")
    nc.vector.memset(ones_f[:], 1.0)
    ones64 = per.tile([64, 1], dt.float32r, name="ones64")
    nc.vector.tensor_copy(ones64[:], ones_f[:64, :])
    # knn1 matmul operands (sq1 folded in chunk-wise)
    lhs_hi1 = ph1.tile([66, N], dt.float32r, name="lhs_hi1")
    nc.vector.tensor_copy(lhs_hi1[:64, :], xhi[:])
    ones2f = tmp0.tile([2, N], dt.float32, name="ones2f")
    nc.vector.memset(ones2f[:], 1.0)
    nc.vector.tensor_copy(lhs_hi1[64:66, :], ones2f[:])
    rhs_hi1 = ph1.tile([66, N], dt.float32r, name="rhs_hi1")
    nc.vector.tensor_scalar_mul(rhs_hi1[:64, :], xhi[:], 2.0)
    psq = tc.alloc_tile_pool(name="psq", bufs=1, space="PSUM")
    sq1p = psq.tile([1, N], dt.float32, name="sq1p", space="PSUM")
    for c in range(4):
        sl = slice(512 * c, 512 * (c + 1))
        nc.tensor.matmul(sq1p[:, sl], ones64[:], xsqhi[:, sl], start=True, stop=False)
        nc.tensor.matmul(sq1p[:, sl], ones64[:], xsqlo[:, sl], start=False, stop=True)
        sc_ = tmp0.tile([1, 512], dt.float32, name="sq1c")
        nc.scalar.copy(sc_[:], sq1p[:, sl])
        sh_ = tmp0.tile([1, 512], dt.float32r, name="sq1ch")
        nc.vector.tensor_copy(sh_[:], sc_[:])
        sl_ = tmp0.tile([1, 512], dt.float32r, name="sq1cl")
        nc.vector.tensor_tensor(out=sl_[:], in0=sc_[:], in1=sh_[:].bitcast(dt.float32),
                                op=ALU.subtract)
        nc.vector.tensor_scalar_mul(rhs_hi1[64:65, sl], sh_[:], -1.0)
        nl_ = tmp0.tile([1, 512], dt.float32r, name="sq1nl")
        nc.vector.tensor_scalar_mul(nl_[:], sl_[:], -1.0)
        nc.sync.dma_start(rhs_hi1[65:66, sl], nl_[:])
    psq.release()

    # conv1 weights
    w1aT = wld.tile([128, 595], dt.float32, name="wf")
    nc.sync.dma_start(w1aT[:64, :152], ins["w1aT"])
    w1aT_hi, w1aT_lo = _round_split(nc, per, w1aT[:64, :152], 64, 152, "w1aT")
    du1T = wld.tile([128, 595], dt.float32, name="wf")
    nc.sync.dma_start(du1T[:64, :152], ins["du1T"])
    du1T_hi, du1T_lo = _round_split(nc, per, du1T[:64, :152], 64, 152, "du1T")
    w1aX = per.tile([P, 152], dt.float32r, name="w1aX")      # [Wlo; Whi]
    nc.vector.tensor_copy(w1aX[:64, :], w1aT_lo[:])
    nc.vector.tensor_copy(w1aX[64:, :], w1aT_hi[:])
    du1X = per.tile([P, 152], dt.float32r, name="du1X")      # [dlo; dhi] pairs lhsx
    nc.vector.tensor_copy(du1X[:64, :], du1T_lo[:])
    nc.vector.tensor_copy(du1X[64:, :], du1T_hi[:])

    wkt1b = []  # conv1b fp32 weights, K-tiles
    for i, rows in enumerate((128, 24)):
        t = per.tile([rows, 181], dt.float32, name=f"w1bT_k{i}")
        nc.sync.dma_start(t[:], ins[f"w1bT_k{i}"])
        wkt1b.append(t)


    # U1[c, n] = sum_ci du1[ci, c] x[ci, n]   (compensated, natural layout)
    rows1a_ = (128, 24)
    u1 = [ph1.tile([r, N], dt.float32, name=f"u1_{i}") for i, r in enumerate(rows1a_)]
    pu = tc.alloc_tile_pool(name="pu", bufs=4, space="PSUM")
    for m, rows in enumerate(rows1a_):
        msl = slice(128 * m, 128 * m + rows)
        for cc in range(4):
            sl = slice(512 * cc, 512 * (cc + 1))
            up = pu.tile([rows, 512], dt.float32, name="up", space="PSUM")
            nc.tensor.matmul(up[:], du1T_hi[:, msl], xhi[:, sl], start=True, stop=False)
            nc.tensor.matmul(up[:], du1X[:, msl], xstack[:, sl], start=False, stop=True)
            nc.scalar.copy(u1[m][:, sl], up[:])
    pu.release()
    tmp0.release()

    # BN params
    def load_vec(nm, rows_list):
        tiles = []
        r0 = 0
        for i, rows in enumerate(rows_list):
            t = per.tile([rows, 1], dt.float32, name=f"{nm}_{i}")
            nc.sync.dma_start(t[:], ins[nm][r0:r0 + rows, :])
            tiles.append(t)
            r0 += rows
        return tiles

    g1a = load_vec("g1a", (128, 24)); b1a = load_vec("be1a", (128, 24))
    g1b = load_vec("g1b", (128, 53)); b1b = load_vec("be1b", (128, 53))
    g2a = load_vec("g2a", (128, 128, 128, 46)); b2a = load_vec("be2a", (128, 128, 128, 46))
    g2b = load_vec("g2b", (128,) * 4); b2b = load_vec("be2b", (128,) * 4)
    g3a = load_vec("g3a", (128,) * 4 + (83,)); b3a = load_vec("be3a", (128,) * 4 + (83,))
    g3b = load_vec("g3b", (128,) * 4); b3b = load_vec("be3b", (128,) * 4)

    # ---------------- kNN (shared machinery) ----------------
    idx1_d = dram.tile([16, P, KNN], dt.uint16, name="idx1_d")
    idx2_d = dram.tile([16, P, KNN], dt.uint16, name="idx2_d")

    def knn_pass(mm_emit, idx_dram, tag, after_tile=None, kp_bufs=2):
        """mm_emit(psum_slice, c) emits matmuls for columns 512c..512c+512."""
        kp = tc.alloc_tile_pool(name=f"knnp_{tag}", bufs=kp_bufs, space="PSUM")
        ks = tc.alloc_tile_pool(name=f"knns_{tag}", bufs=2)
        for t in range(16):
            pe = kp.tile([P, N], dt.float32, name="pe", space="PSUM")
            for c in range(4):
                mm_emit(pe, t, c)
            q = ks.tile([P, N], dt.float32, name="q")
            nc.scalar.copy(q[:], pe[:])
            v8 = ks.tile([P, 8], dt.float32, name="v8")
            i8 = ks.tile([P, 8], dt.uint16, name="i8")
            nc.vector.max(out=v8[:], in_=q[:])
            nc.vector.max_index(out=i8[:], in_max=v8[:], in_values=q[:])
            nc.vector.match_replace(out=q[:], in_to_replace=v8[:], in_values=q[:],
                                    imm_value=-1e30)
            v8b = ks.tile([P, 8], dt.float32, name="v8b")
            i8b = ks.tile([P, 8], dt.uint16, name="i8b")
            nc.vector.max(out=v8b[:], in_=q[:])
            nc.vector.max_index(out=i8b[:], in_max=v8b[:], in_values=q[:])
            idx = ks.tile([P, KNN], dt.uint16, name="idx")
            nc.vector.tensor_copy(idx[:, 0:7], i8[:, 1:8])
            nc.vector.tensor_copy(idx[:, 7:10], i8b[:, 0:3])
            nc.sync.dma_start(idx_dram[t], idx[:])
            if after_tile is not None:
                after_tile(t)
        kp.release()
        ks.release()

    def build_wrapped(idx_dram, tag):
        """DRAM flat idx (point-major) -> wrapped [128, S/16] uint16."""
        iw = per.tile([P, S // 16], dt.uint16, name=f"iw_{tag}")
        flat = idx_dram[:].rearrange("a p k -> (a p k)")
        nc.sync.dma_start(iw[0:16, :], flat.rearrange("(f p) -> p f", p=16))
        for g in range(1, 8):
            nc.sync.dma_start(iw[16 * g:16 * (g + 1), :], iw[0:16, :])
        return iw

    # knn1 matmuls
    def mm1(pe, t, c):
        msl = slice(128 * t, 128 * (t + 1))
        csl = slice(512 * c, 512 * (c + 1))
        nc.tensor.matmul(pe[:, csl], lhs_hi1[:, msl], rhs_hi1[:, csl],
                         start=True, stop=False)
        nc.tensor.matmul(pe[:, csl], lhsx[:, msl], rhsx[:, csl],
                         start=False, stop=True)

    # ---------------- block 1, interleaved with knn1 ----------------
    # y1a[(n,j)] = W1a @ (nb - cen) + W1b @ cen ; stats; keep for BN.
    # Each knn1 tile's top-k feeds its 4 stats chunks immediately, so the
    # gather-paced loop overlaps the DVE top-k of later tiles.
    y1a_d = [dram.tile([P, S], dt.float32, name="y1a_d0"),
             dram.tile([24, S], dt.float32, name="y1a_d1")]
    rows1a = (128, 24)
    sum1a = [ph1.tile([r, NCHUNK], dt.float32, name=f"s1a_{i}") for i, r in enumerate(rows1a)]
    sq1a = [ph1.tile([r, NCHUNK], dt.float32, name=f"q1a_{i}") for i, r in enumerate(rows1a)]
    # b1 pools pre-allocated so the gather stream overlaps knn1; iw1 built
    # per-tile so gather t only depends on top-k tile t
    iw1 = per.tile([P, S // 16], dt.uint16, name="iw_k1")
    b1p = tc.alloc_tile_pool(name="b1p", bufs=1, space="PSUM")
    b1s = tc.alloc_tile_pool(name="b1s", bufs=3)
    knn_pass(mm1, idx1_d, "k1", kp_bufs=1)

    b1q = {}
    def b1_gather(t):
        cols = slice(80 * t, 80 * (t + 1))
        flat_t = idx1_d[t].rearrange("p k -> (p k)")
        nc.sync.dma_start(iw1[0:16, cols], flat_t.rearrange("(f p) -> p f", p=16))
        nc.sync.dma_start(iw1[16:32, cols], iw1[0:16, cols])
        nc.sync.dma_start(iw1[32:64, cols], iw1[0:32, cols])
        nc.sync.dma_start(iw1[64:128, cols], iw1[0:64, cols])
        nbr = b1s.tile([P, CB], dt.float32, name="nbr")
        for g in range(2):
            nc.gpsimd.indirect_copy(out=nbr[:, 640 * g:640 * (g + 1)],
                                    data=xstack[:].bitcast(dt.float32),
                                    idxs=iw1[:, 80 * t + 40 * g:80 * t + 40 * (g + 1)],
                                    i_know_ap_gather_is_preferred=True)
        nbxt = b1s.tile([P, CB], dt.float32r, name="nbxt")
        nc.scalar.copy(nbxt[:], nbr[:])
        b1q[t] = nbxt

    b1_gather(0)
    for t in range(NCHUNK):
        if t + 1 < NCHUNK:
            b1_gather(t + 1)
        nbx = b1q.pop(t)[:]
        csl = slice(CB * t, CB * (t + 1))
        for m, rows in enumerate(rows1a):
            msl = slice(128 * m, 128 * m + rows)
            yp = b1p.tile([P, CB], dt.float32, name="yp", space="PSUM")
            for c0, c1 in CSL:
                nc.tensor.matmul(yp[:rows, c0:c1], w1aT_hi[:, msl],
                                 nbx[0:64, c0:c1], start=True, stop=False)
                nc.tensor.matmul(yp[:rows, c0:c1], w1aX[:, msl],
                                 nbx[:, c0:c1], start=False, stop=True)
            uview = u1[m][:, 128 * t:128 * (t + 1), None].to_broadcast(
                [rows, 128, KNN])
            ob = b1s.tile([rows, CB], dt.float32, name="ob1")
            nc.vector.scalar_tensor_tensor(
                out=ob[:].rearrange("p (n k) -> p n k", k=KNN),
                in0=yp[:rows, :].rearrange("p (n k) -> p n k", k=KNN), scalar=1.0,
                in1=uview, op0=ALU.mult, op1=ALU.add,
                accum_out=sum1a[m][:, t:t + 1])
            nc.sync.dma_start(y1a_d[m][:, csl], ob[:])
            tr = b1t.tile([rows, CB], dt.bfloat16, name="tr")
            nc.scalar.activation(tr[:], ob[:], AF.Square,
                                 accum_out=sq1a[m][:, t:t + 1])
    b1s.release()
    b1p.release()
    k2pool = tc.alloc_tile_pool(name="k2pool", bufs=1)

    s1 = [_acc_reduce(nc, per, sum1a[m][:], rows1a[m], f"s1r{m}") for m in range(2)]
    q1 = [_acc_reduce(nc, per, sq1a[m][:], rows1a[m], f"q1r{m}") for m in range(2)]

    # h1 load prefetch issued before the AllReduce so DMA overlaps it
    sumh1 = [ph1.tile([r, NCHUNK], dt.float32, name=f"sh1_{i}") for i, r in enumerate(rows1a)]
    rows1b = (128, 53)
    m1 = [k2pool.tile([r, N], dt.float32, name=f"m1_{i}") for i, r in enumerate(rows1b)]
    sq1b = [ph1.tile([r, NCHUNK], dt.float32, name=f"q1b_{i}") for i, r in enumerate(rows1b)]
    b2pp = tc.alloc_tile_pool(name="b2pp", bufs=2, space="PSUM")
    h1l = tc.alloc_tile_pool(name="h1l", bufs=3)

    h1q = {}
    def h1_load(c):
        csl = slice(CB * c, CB * (c + 1))
        pair = []
        for m, rows in enumerate(rows1a):
            ld = h1l.tile([rows, CB], dt.float32, name=f"h1ld{m}")
            nc.sync.dma_start(ld[:], y1a_d[m][:, csl])
            pair.append(ld)
        h1q[c] = pair

    h1_load(0)
    ar1 = _allreduce(nc, dram, per, [s1[0][:], s1[1][:]], [q1[0][:], q1[1][:]],
                     rows1a, 152, "ar1")
    sb1a = [_bn_coeffs(nc, per, ar1[m][0][:], ar1[m][1][:], g1a[m][:], b1a[m][:],
                       rows1a[m], CNT, f"bn1a{m}") for m in range(2)]
    for c in range(NCHUNK):
        if c + 1 < NCHUNK:
            h1_load(c + 1)
        csl = slice(CB * c, CB * (c + 1))
        h1c = h1q.pop(c)
        for m, rows in enumerate(rows1a):
            ld = h1c[m]
            nc.scalar.activation(ld[:], ld[:], AF.Prelu,
                                 bias=sb1a[m][1][:], scale=sb1a[m][0][:], alpha=0.2,
                                 accum_out=sumh1[m][:, c:c + 1])
        for m, rows in enumerate(rows1b):
            msl = slice(128 * m, 128 * m + rows)
            yp = b2pp.tile([P, CB], dt.float32, name="y1bp", space="PSUM")
            for c0, c1 in CSL:
                nc.tensor.matmul(yp[:rows, c0:c1], wkt1b[0][:, msl],
                                 h1c[0][:, c0:c1], start=True, stop=False)
                nc.tensor.matmul(yp[:rows, c0:c1], wkt1b[1][:, msl],
                                 h1c[1][:, c0:c1], start=False, stop=True)
            tr = b1t.tile([rows, CB], dt.bfloat16, name="tr")
            nc.scalar.activation(tr[:], yp[:rows, :], AF.Square,
                                 accum_out=sq1b[m][:, c:c + 1])
            nc.vector.tensor_reduce(out=m1[m][:, 128 * c:128 * (c + 1)],
                                    in_=yp[:rows, :].rearrange("p (n k) -> p n k", k=KNN),
                                    axis=AX.X, op=ALU.max)
    b2pp.release()
    h1l.release()

    # sum(y1b) = W1b' @ sum(h1)
    sh1 = [_acc_reduce(nc, per, sumh1[m][:], rows1a[m], f"sh1r{m}") for m in range(2)]
    ps1b = tc.alloc_tile_pool(name="ps1b", bufs=1, space="PSUM")
    s1b = []
    for m, rows in enumerate(rows1b):
        msl = slice(128 * m, 128 * m + rows)
        sp = ps1b.tile([rows, 1], dt.float32, name=f"s1bp{m}", space="PSUM")
        nc.tensor.matmul(sp[:], wkt1b[0][:, msl], sh1[0][:], start=True, stop=False)
        nc.tensor.matmul(sp[:], wkt1b[1][:, msl], sh1[1][:], start=False, stop=True)
        st = per.tile([rows, 1], dt.float32, name=f"s1b_{m}")
        nc.scalar.copy(st[:], sp[:])
        s1b.append(st)
    q1b = [_acc_reduce(nc, per, sq1b[m][:], rows1b[m], f"q1br{m}") for m in range(2)]
    ps1b.release()
    ar2 = _allreduce(nc, dram, per, [s1b[0][:], s1b[1][:]], [q1b[0][:], q1b[1][:]],
                     rows1b, 181, "ar2")
    sb1b = [_bn_coeffs(nc, per, ar2[m][0][:], ar2[m][1][:], g1b[m][:], b1b[m][:],
                       rows1b[m], CNT, f"bn1b{m}") for m in range(2)]

    # x1 = prelu(bn(m1)) -> hi/lo splits + sq2, all chunk-wise
    pk2op = tc.alloc_tile_pool(name="pk2op", bufs=1)   # knn2 operands (freed w/ k2pool)
    ones53 = per.tile([53, 1], dt.float32r, name="ones53")
    nc.vector.tensor_copy(ones53[:], ones_f[:53, :])
    ones128 = per.tile([128, 1], dt.float32r, name="ones128")
    nc.vector.tensor_copy(ones128[:], ones_f[:])
    x1hi = [per.tile([r, N], dt.float32r, name=f"x1s{m}_hi") for m, r in enumerate(rows1b)]
    x1lo = [pk2op.tile([r, N], dt.bfloat16, name=f"x1s{m}_lo") for m, r in enumerate(rows1b)]
    r2h_k1 = pk2op.tile([55, N], dt.float32r, name="r2h_k1")
    psq2 = tc.alloc_tile_pool(name="psq2", bufs=1, space="PSUM")
    sqt = tc.alloc_tile_pool(name="sqt", bufs=1)
    sq2p = psq2.tile([1, N], dt.float32, name="sq2p", space="PSUM")
    for c in range(4):
        sl = slice(512 * c, 512 * (c + 1))
        first = True
        for m, rows in enumerate(rows1b):
            x1c = sqt.tile([rows, 512], dt.float32, name="x1c")
            nc.scalar.activation(x1c[:], m1[m][:, sl], AF.Prelu, bias=sb1b[m][1][:],
                                 scale=sb1b[m][0][:], alpha=0.2)
            nc.vector.tensor_copy(x1hi[m][:, sl], x1c[:])
            nc.vector.tensor_tensor(out=x1lo[m][:, sl], in0=x1c[:],
                                    in1=x1hi[m][:, sl].bitcast(dt.float32),
                                    op=ALU.subtract)
            t = sqt.tile([rows, 512], dt.float32, name="x1sqc")
            nc.vector.tensor_tensor(out=t[:], in0=x1c[:], in1=x1c[:], op=ALU.mult)
            h = sqt.tile([rows, 512], dt.float32r, name="x1sqh")
            nc.vector.tensor_copy(h[:], t[:])
            l = sqt.tile([rows, 512], dt.float32r, name="x1sql")
            nc.vector.tensor_tensor(out=l[:], in0=t[:], in1=h[:].bitcast(dt.float32),
                                    op=ALU.subtract)
            on = ones128 if rows == 128 else ones53
            nc.tensor.matmul(sq2p[:, sl], on[:], h[:], start=first, stop=False)
            first = False
            nc.tensor.matmul(sq2p[:, sl], on[:], l[:],
                             start=False, stop=(m == 1))
        sc_ = sqt.tile([1, 512], dt.float32, name="sq2c")
        nc.scalar.copy(sc_[:], sq2p[:, sl])
        sh_ = sqt.tile([1, 512], dt.float32r, name="sq2ch")
        nc.vector.tensor_copy(sh_[:], sc_[:])
        sl_ = sqt.tile([1, 512], dt.float32r, name="sq2cl")
        nc.vector.tensor_tensor(out=sl_[:], in0=sc_[:], in1=sh_[:].bitcast(dt.float32),
                                op=ALU.subtract)
        nh_ = sqt.tile([1, 512], dt.float32r, name="sq2nh")
        nc.vector.tensor_scalar_mul(nh_[:], sh_[:], -1.0)
        nc.sync.dma_start(r2h_k1[53:54, sl], nh_[:])
        nl_ = sqt.tile([1, 512], dt.float32r, name="sq2nl")
        nc.vector.tensor_scalar_mul(nl_[:], sl_[:], -1.0)
        nc.sync.dma_start(r2h_k1[54:55, sl], nl_[:])
    psq2.release()
    sqt.release()

    # knn2 operands: lhsT reuses x1hi/x1lo; only scaled rhs tiles are built
    l2h_k1 = pk2op.tile([55, N], dt.float32r, name="l2h_k1")    # x1hi 128:181 + 2 ones
    nc.vector.tensor_copy(l2h_k1[0:53, :], x1hi[1][:])
    ones2f2 = pk2op.tile([2, N], dt.float32, name="ones2f2")
    nc.vector.memset(ones2f2[:], 1.0)
    ones2n = pk2op.tile([2, N], dt.float32r, name="ones2n")
    nc.vector.tensor_copy(ones2n[:], ones2f2[:])
    nc.sync.dma_start(l2h_k1[53:55, :], ones2n[:])
    r2h_k0 = pk2op.tile([P, N], dt.float32r, name="r2h_k0")     # 2*x1hi[0]
    nc.vector.tensor_scalar_mul(r2h_k0[:], x1hi[0][:], 2.0)
    nc.vector.tensor_scalar_mul(r2h_k1[0:53, :], x1hi[1][:], 2.0)
    r2l0 = pk2op.tile([P, N], dt.bfloat16, name="r2l0")         # 2*x1lo[0]
    nc.vector.tensor_scalar_mul(r2l0[:], x1lo[0][:], 2.0)
    r2l1 = pk2op.tile([53, N], dt.bfloat16, name="r2l1")        # 2*x1lo[1]
    nc.vector.tensor_scalar_mul(r2l1[:], x1lo[1][:], 2.0)
    # bf16 copies of hi operands for the lo-correction matmuls (1 cyc/col)
    hiB0 = pk2op.tile([P, N], dt.bfloat16, name="hiB0")
    nc.vector.tensor_copy(hiB0[:], x1hi[0][:].bitcast(dt.float32))
    hiB1 = pk2op.tile([53, N], dt.bfloat16, name="hiB1")
    nc.vector.tensor_copy(hiB1[:], x1hi[1][:].bitcast(dt.float32))
    r2hB0 = pk2op.tile([P, N], dt.bfloat16, name="r2hB0")
    nc.vector.tensor_copy(r2hB0[:], r2h_k0[:].bitcast(dt.float32))
    r2hB1 = pk2op.tile([53, N], dt.bfloat16, name="r2hB1")
    nc.vector.tensor_copy(r2hB1[:], r2h_k1[0:53, :].bitcast(dt.float32))

    def mm2(pe, t, c):
        msl = slice(128 * t, 128 * (t + 1))
        csl = slice(512 * c, 512 * (c + 1))
        nc.tensor.matmul(pe[:, csl], x1hi[0][:, msl], r2h_k0[:, csl], start=True, stop=False)
        nc.tensor.matmul(pe[:, csl], l2h_k1[:, msl], r2h_k1[:, csl], start=False, stop=False)
        nc.tensor.matmul(pe[:, csl], hiB0[:, msl], r2l0[:, csl], start=False, stop=False)
        nc.tensor.matmul(pe[:, csl], hiB1[:, msl], r2l1[:, csl], start=False, stop=False)
        nc.tensor.matmul(pe[:, csl], x1lo[0][:, msl], r2hB0[:, csl], start=False, stop=False)
        nc.tensor.matmul(pe[:, csl], x1lo[1][:, msl], r2hB1[:, csl], start=False, stop=True)

    iw2 = per.tile([P, S // 16], dt.uint16, name="iw_k2")
    iw2d = per.tile([P, S // 16], dt.uint16, name="iw2d")
    knn_pass(mm2, idx2_d, "k2")
    pk2op.release()
    k2pool.release()
    ph1.release()

    # ---------------- block 2 ----------------
    # gather pools first: they reuse ph1/k2pool bytes whose users finished
    # before knn2, so the b2a gather stream can overlap knn2
    b2g = tc.alloc_tile_pool(name="b2g", bufs=1)
    gxi = b2g.tile([P, 2 * N], dt.bfloat16, name="gxi")
    _g3 = gxi[:].rearrange("p (n two) -> p n two", two=2)
    nc.scalar.copy(_g3[:, :, 0], x1hi[0][:].bitcast(dt.float32))
    nc.scalar.copy(_g3[:, :, 1], x1hi[0][:].bitcast(dt.float32))
    nc.scalar.copy(_g3[0:53, :, 1], x1hi[1][:].bitcast(dt.float32))
    b2s = tc.alloc_tile_pool(name="b2s", bufs=4)

    # conv2b weights early so their DMA overlaps b2a / AR3
    b2bpool = tc.alloc_tile_pool(name="b2bpool", bufs=1)
    w2bT_r, w2bT_b = [], []
    for i, rows in enumerate((128, 128, 128, 46)):
        wt = wld.tile([128, 595], dt.float32, name="wf")
        nc.sync.dma_start(wt[:rows, :512], ins[f"w2bT_k{i}"])
        wr = b2bpool.tile([rows, 512], dt.float32r, name=f"w2bT_r{i}")
        nc.vector.tensor_copy(wr[:], wt[:rows, :512])
        w2bT_r.append(wr)
        wb = b2bpool.tile([rows, 512], dt.bfloat16, name=f"w2bT_b{i}")
        nc.vector.tensor_copy(wb[:], wt[:rows, :512])
        w2bT_b.append(wb)

    # U2[c, n] = sum_ci du2[ci, c] x1[ci, n]   (plain f32r)
    b2pool = tc.alloc_tile_pool(name="b2pool", bufs=1)
    w2aT_r, du2T_r = [], []
    for i, rows in enumerate((128, 53)):
        wt = wld.tile([128, 595], dt.float32, name="wf")
        nc.sync.dma_start(wt[:rows, :430], ins[f"w2aT_k{i}"])
        wr = b2pool.tile([rows, 430], dt.float32r, name=f"w2aT_r{i}")
        nc.vector.tensor_copy(wr[:], wt[:rows, :430])
        w2aT_r.append(wr)
        ddt = wld.tile([128, 595], dt.float32, name="wf")
        nc.sync.dma_start(ddt[:rows, :430], ins[f"du2T_k{i}"])
        dr = b2pool.tile([rows, 430], dt.float32r, name=f"du2T_r{i}")
        nc.vector.tensor_copy(dr[:], ddt[:rows, :430])
        du2T_r.append(dr)
    rows2a_ = (128, 128, 128, 46)
    u2 = [b2pool.tile([r, N], dt.float32, name=f"u2_{i}") for i, r in enumerate(rows2a_)]
    pu2 = tc.alloc_tile_pool(name="pu2", bufs=4, space="PSUM")
    for m, rows in enumerate(rows2a_):
        msl = slice(128 * m, 128 * m + rows)
        for cc in range(4):
            sl = slice(512 * cc, 512 * (cc + 1))
            up = pu2.tile([rows, 512], dt.float32, name="up2", space="PSUM")
            nc.tensor.matmul(up[:], du2T_r[0][:, msl], x1hi[0][:, sl], start=True, stop=False)
            nc.tensor.matmul(up[:], du2T_r[1][:, msl], x1hi[1][:, sl], start=False, stop=True)
            nc.scalar.copy(u2[m][:, sl], up[:])
    pu2.release()

    rows2a = (128, 128, 128, 46)
    y2a_d = [dram.tile([r, S], dt.bfloat16, name=f"y2a_d{i}") for i, r in enumerate(rows2a)]
    sum2a = [b2pool.tile([r, NCHUNK], dt.float32, name=f"s2a_{i}") for i, r in enumerate(rows2a)]
    sq2a = [b2pool.tile([r, NCHUNK], dt.float32, name=f"q2a_{i}") for i, r in enumerate(rows2a)]

    w2aT_b = []
    for i, rows in enumerate((128, 53)):
        wb = b2pool.tile([rows, 430], dt.bfloat16, name=f"w2aT_b{i}")
        nc.vector.tensor_copy(wb[:], w2aT_r[i][:].bitcast(dt.float32))
        w2aT_b.append(wb)

    b2p = tc.alloc_tile_pool(name="b2p", bufs=2, space="PSUM")
    b2o = tc.alloc_tile_pool(name="b2o", bufs=3)
    gxi3 = gxi[:].rearrange("p (n two) -> p n two", two=2)
    b2q = {}
    def b2_gather(c):
        cols = slice(80 * c, 80 * (c + 1))
        flat_t = idx2_d[c].rearrange("p k -> (p k)")
        nc.sync.dma_start(iw2[0:16, cols], flat_t.rearrange("(f p) -> p f", p=16))
        nc.sync.dma_start(iw2[16:32, cols], iw2[0:16, cols])
        nc.sync.dma_start(iw2[32:64, cols], iw2[0:32, cols])
        nc.sync.dma_start(iw2[64:128, cols], iw2[0:64, cols])
        nc.scalar.mul(iw2d[:, cols], iw2[:, cols], 2.0)
        nbi = b2s.tile([P, 2 * CB], dt.bfloat16, name="nbi")
        nbi3 = nbi[:].rearrange("p (n two) -> p n two", two=2)
        for e0, e1 in ((0, 512), (512, 1024), (1024, 1280)):
            nc.gpsimd.indirect_copy(
                out=nbi3[:, e0:e1, :], data=gxi3,
                idxs=iw2d[:, 80 * c + e0 // 16:80 * c + e1 // 16],
                i_know_ap_gather_is_preferred=True)
        nb0r = b2s.tile([P, CB], dt.bfloat16, name="nb0r")
        nc.vector.tensor_copy(nb0r[:], nbi3[:, :, 0])
        nb1r = b2s.tile([P, CB], dt.bfloat16, name="nb1r")
        nc.vector.tensor_copy(nb1r[:], nbi3[:, :, 1])
        b2q[c] = (nb0r, nb1r)

    for _pc in range(4):
        b2_gather(_pc)
    for c in range(NCHUNK):
        if c + 4 < NCHUNK:
            b2_gather(c + 4)
        nb0r, nb1r = b2q.pop(c)
        for m, rows in enumerate(rows2a):
            msl = slice(128 * m, 128 * m + rows)
            yp = b2p.tile([P, CB], dt.float32, name="y2ap", space="PSUM")
            for c0, c1 in CSL:
                nc.tensor.matmul(yp[:rows, c0:c1], w2aT_b[0][:, msl],
                                 nb0r[:, c0:c1], start=True, stop=False)
                nc.tensor.matmul(yp[:rows, c0:c1], w2aT_b[1][:, msl],
                                 nb1r[0:53, c0:c1], start=False, stop=True)
            uview = u2[m][:, 128 * c:128 * (c + 1), None].to_broadcast(
                [rows, 128, KNN])
            ob = b2o.tile([rows, CB], dt.bfloat16, name="ob2")
            nc.vector.scalar_tensor_tensor(
                out=ob[:].rearrange("p (n k) -> p n k", k=KNN),
                in0=yp[:rows, :].rearrange("p (n k) -> p n k", k=KNN), scalar=1.0,
                in1=uview, op0=ALU.mult, op1=ALU.add,
                accum_out=sum2a[m][:, c:c + 1])
            nc.sync.dma_start(y2a_d[m][:, CB * c:CB * (c + 1)], ob[:])
            tr = b1t.tile([rows, CB], dt.bfloat16, name="tr")
            nc.scalar.activation(tr[:], ob[:], AF.Square,
                                 accum_out=sq2a[m][:, c:c + 1])
    b2p.release()
    b2o.release()

    s2 = [_acc_reduce(nc, per, sum2a[m][:], rows2a[m], f"s2r{m}") for m in range(4)]
    q2 = [_acc_reduce(nc, per, sq2a[m][:], rows2a[m], f"q2r{m}") for m in range(4)]
    b2pool.release()

    # block-3 weights early (LIFO: pw3 below b3p/b3l/c3t/px2r); DMA overlaps conv2b
    pw3 = tc.alloc_tile_pool(name="pw3", bufs=1)
    w3aT_r = []
    rows3a_k = (128, 53, 128, 128, 128, 128)
    for i, rows in enumerate(rows3a_k):
        wt = wld.tile([128, 595], dt.float32, name="wf")
        nc.sync.dma_start(wt[:rows, :595], ins[f"w3aT_k{i}"])
        wr = pw3.tile([rows, 595], dt.bfloat16, name=f"w3aT_r{i}")
        nc.vector.tensor_copy(wr[:], wt[:rows, :595])
        w3aT_r.append(wr)
    w3bT_r = []
    rows3b_k = (128, 128, 128, 128, 83)
    for i, rows in enumerate(rows3b_k):
        wt = wld.tile([128, 595], dt.float32, name="wf")
        nc.sync.dma_start(wt[:rows, :512], ins[f"w3bT_k{i}"])
        wr = pw3.tile([rows, 512], dt.bfloat16, name=f"w3bT_r{i}")
        nc.vector.tensor_copy(wr[:], wt[:rows, :512])
        w3bT_r.append(wr)
    pb3 = tc.alloc_tile_pool(name="pb3", bufs=1)   # y3a/y3b bf16 residency

    rows2b = (128, 128, 128, 128)
    sumh2 = [b2bpool.tile([r, NCHUNK], dt.float32, name=f"sh2_{i}") for i, r in enumerate(rows2a)]
    sq2b = [b2bpool.tile([r, NCHUNK], dt.float32, name=f"q2b_{i}") for i, r in enumerate(rows2b)]
    m2 = [b2bpool.tile([r, N], dt.bfloat16, name=f"m2_{i}") for i, r in enumerate(rows2b)]

    b3p = tc.alloc_tile_pool(name="b3p", bufs=2, space="PSUM")
    b3l = tc.alloc_tile_pool(name="b3l", bufs=2)

    c2q = {}
    def c2_load(c):
        csl = slice(CB * c, CB * (c + 1))
        grp = []
        for m, rows in enumerate(rows2a):
            ld = b3l.tile([rows, CB], dt.bfloat16, name=f"ld{m}")
            nc.sync.dma_start(ld[:], y2a_d[m][:, csl])
            grp.append(ld)
        c2q[c] = grp

    c2_load(0)
    ar3 = _allreduce(nc, dram, per, [s[:] for s in s2], [q[:] for q in q2],
                     rows2a, 430, "ar3")
    sb2a = [_bn_coeffs(nc, per, ar3[m][0][:], ar3[m][1][:], g2a[m][:], b2a[m][:],
                       rows2a[m], CNT, f"bn2a{m}") for m in range(4)]
    for c in range(NCHUNK):
        if c + 1 < NCHUNK:
            c2_load(c + 1)
        csl = slice(CB * c, CB * (c + 1))
        h2 = c2q.pop(c)
        for m, rows in enumerate(rows2a):
            ld = h2[m]
            nc.scalar.activation(ld[:], ld[:], AF.Prelu, bias=sb2a[m][1][:],
                                 scale=sb2a[m][0][:], alpha=0.2,
                                 accum_out=sumh2[m][:, c:c + 1])
        for m, rows in enumerate(rows2b):
            msl = slice(128 * m, 128 * (m + 1))
            yp = b3p.tile([P, CB], dt.float32, name="y2bp", space="PSUM")
            for c0, c1 in CSL:
                for k in range(4):
                    kr = (128, 128, 128, 46)[k]
                    nc.tensor.matmul(yp[:rows, c0:c1], w2bT_b[k][:, msl],
                                     h2[k][0:kr, c0:c1],
                                     start=(k == 0), stop=(k == 3))
            tr = b1t.tile([rows, CB], dt.bfloat16, name="tr")
            nc.scalar.activation(tr[:], yp[:rows, :], AF.Square,
                                 accum_out=sq2b[m][:, c:c + 1])
            nc.vector.tensor_reduce(out=m2[m][:, 128 * c:128 * (c + 1)],
                                    in_=yp[:rows, :].rearrange("p (n k) -> p n k", k=KNN),
                                    axis=AX.X, op=ALU.max)
    b3p.release()
    b3l.release()

    sh2 = [_acc_reduce(nc, per, sumh2[m][:], rows2a[m], f"sh2r{m}") for m in range(4)]
    sh2r = []
    for m in range(4):
        t = per.tile([rows2a[m], 4], dt.float32r, name=f"sh2rr{m}")
        nc.vector.tensor_copy(t[:], sh2[m][:].to_broadcast([rows2a[m], 4]))
        sh2r.append(t)
    ps2b = tc.alloc_tile_pool(name="ps2b", bufs=1, space="PSUM")
    s2b = []
    for m in range(4):
        msl = slice(128 * m, 128 * (m + 1))
        sp = ps2b.tile([128, 4], dt.float32, name=f"s2bp{m}", space="PSUM")
        for k in range(4):
            kr = (128, 128, 128, 46)[k]
            nc.tensor.matmul(sp[:], w2bT_r[k][:, msl], sh2r[k][0:kr, :],
                             start=(k == 0), stop=(k == 3))
        st = per.tile([128, 1], dt.float32, name=f"s2b_{m}")
        nc.scalar.copy(st[:], sp[:, 0:1])
        s2b.append(st)
    ps2b.release()
    q2b = [_acc_reduce(nc, per, sq2b[m][:], 128, f"q2br{m}") for m in range(4)]
    ar4 = _allreduce(nc, dram, per, [s[:] for s in s2b], [q[:] for q in q2b],
                     rows2b, 512, "ar4")
    sb2b = [_bn_coeffs(nc, per, ar4[m][0][:], ar4[m][1][:], g2b[m][:], b2b[m][:],
                       128, CNT, f"bn2b{m}") for m in range(4)]

    # x2 = prelu(bn(m2)) -> bf16; x1 recast to bf16 for block 3
    c3t = tc.alloc_tile_pool(name="c3t", bufs=2)
    px2r = tc.alloc_tile_pool(name="px2r", bufs=1)
    x2r = [px2r.tile([128, N], dt.bfloat16, name=f"x2r_{i}") for i in range(4)]
    for m in range(4):
        nc.scalar.activation(x2r[m][:], m2[m][:], AF.Prelu, bias=sb2b[m][1][:],
                             scale=sb2b[m][0][:], alpha=0.2)
    xb1 = [px2r.tile([r, N], dt.bfloat16, name=f"xb1_{i}")
           for i, r in enumerate((128, 53))]
    for i in range(2):
        nc.vector.tensor_copy(xb1[i][:], x1hi[i][:].bitcast(dt.float32))

    # ---------------- block 3 ----------------
    # y3a / y3b stay SBUF-resident in bf16 (post-x1 path tolerates bf16)
    xc_k = [xb1[0], xb1[1], x2r[0], x2r[1], x2r[2], x2r[3]]
    rows3a = (128, 128, 128, 128, 83)
    y3a_s = [pb3.tile([r, N], dt.bfloat16, name=f"y3a_s{i}") for i, r in enumerate(rows3a)]
    sum3a = [per.tile([r, 4], dt.float32, name=f"s3a_{i}") for i, r in enumerate(rows3a)]
    sq3a = [per.tile([r, 4], dt.float32, name=f"q3a_{i}") for i, r in enumerate(rows3a)]
    c3p = tc.alloc_tile_pool(name="c3p", bufs=4, space="PSUM")
    for c in range(4):
        csl = slice(512 * c, 512 * (c + 1))
        for m, rows in enumerate(rows3a):
            msl = slice(128 * m, 128 * m + rows)
            yp = c3p.tile([128, 512], dt.float32, name="y3ap", space="PSUM")
            for k in range(6):
                nc.tensor.matmul(yp[:rows, :], w3aT_r[k][:, msl], xc_k[k][:, csl],
                                 start=(k == 0), stop=(k == 5))
            nc.scalar.activation(y3a_s[m][:, csl], yp[:rows, :], AF.Copy,
                                 accum_out=sum3a[m][:, c:c + 1])
            tr = c3t.tile([rows, 512], dt.bfloat16, name="tr")
            nc.scalar.activation(tr[:], yp[:rows, :], AF.Square,
                                 accum_out=sq3a[m][:, c:c + 1])
    c3p.release()
    px2r.release()

    s3 = [_acc_reduce(nc, per, sum3a[m][:], rows3a[m], f"s3r{m}") for m in range(5)]
    q3 = [_acc_reduce(nc, per, sq3a[m][:], rows3a[m], f"q3r{m}") for m in range(5)]
    ar5 = _allreduce(nc, dram, per, [s[:] for s in s3], [q[:] for q in q3],
                     rows3a, 640, "ar5")
    sb3a = [_bn_coeffs(nc, per, ar5[m][0][:], ar5[m][1][:], g3a[m][:], b3a[m][:],
                       rows3a[m], CNT3, f"bn3a{m}") for m in range(5)]

    rows3b = (128, 128, 128, 128)
    y3b_s = [pb3.tile([r, N], dt.bfloat16, name=f"y3b_s{i}") for i, r in enumerate(rows3b)]
    sum3b = [per.tile([r, 4], dt.float32, name=f"s3b_{i}") for i, r in enumerate(rows3b)]
    sq3b = [per.tile([r, 4], dt.float32, name=f"q3b_{i}") for i, r in enumerate(rows3b)]
    c4p = tc.alloc_tile_pool(name="c4p", bufs=2, space="PSUM")
    h3l = tc.alloc_tile_pool(name="h3l", bufs=6)
    for c in range(4):
        csl = slice(512 * c, 512 * (c + 1))
        h3c = []
        for k, kr in enumerate(rows3a):
            ld = h3l.tile([kr, 512], dt.bfloat16, name="h3ld")
            nc.scalar.activation(ld[:], y3a_s[k][:, csl], AF.Prelu,
                                 bias=sb3a[k][1][:], scale=sb3a[k][0][:], alpha=0.2)
            h3c.append(ld)
        for m, rows in enumerate(rows3b):
            msl = slice(128 * m, 128 * (m + 1))
            yp = c4p.tile([rows, 512], dt.float32, name=f"y3bp{m}", space="PSUM")
            for k in range(5):
                nc.tensor.matmul(yp[:], w3bT_r[k][:, msl], h3c[k][:],
                                 start=(k == 0), stop=(k == 4))
            nc.scalar.activation(y3b_s[m][:, csl], yp[:], AF.Copy,
                                 accum_out=sum3b[m][:, c:c + 1])
            tr = c3t.tile([rows, 512], dt.bfloat16, name="tr")
            nc.scalar.activation(tr[:], yp[:], AF.Square,
                                 accum_out=sq3b[m][:, c:c + 1])
    c4p.release()
    h3l.release()
    c3t.release()

    s4 = [_acc_reduce(nc, per, sum3b[m][:], 128, f"s4r{m}") for m in range(4)]
    q4 = [_acc_reduce(nc, per, sq3b[m][:], 128, f"q4r{m}") for m in range(4)]
    ar6 = _allreduce(nc, dram, per, [s[:] for s in s4], [q[:] for q in q4],
                     rows3b, 512, "ar6")
    sb3b = [_bn_coeffs(nc, per, ar6[m][0][:], ar6[m][1][:], g3b[m][:], b3b[m][:],
                       128, CNT3, f"bn3b{m}") for m in range(4)]

    out_d = nc._ext_outputs["out"]
    for m in range(4):
        for c in range(4):
            csl = slice(512 * c, 512 * (c + 1))
            ot = oseg.tile([128, 512], dt.float32, name="ot")
            nc.scalar.activation(ot[:], y3b_s[m][:, csl], AF.Prelu,
                                 bias=sb3b[m][1][:], scale=sb3b[m][0][:], alpha=0.2)
            nc.sync.dma_start(out_d[128 * m:128 * (m + 1), csl], ot[:])
    pb3.release()
    pw3.release()
    b2bpool.release()
    b2s.release()
    b2g.release()
    wld.release()
    oseg.release()
    b1t.release()
    per.release()


# ------------------------------------------------------------------ host side
_CACHE = {}


def _build():
    _install_bassfix()
    nc = bass.Bass("TRN2", target_bir_lowering=False, debug=False,
                   num_devices=NCORES)
    in_specs = {
        "x": (64, N), "w1aT": (64, 152), "du1T": (64, 152),
        "w1bT_k0": (128, 181), "w1bT_k1": (24, 181),
        "w2aT_k0": (128, 430), "w2aT_k1": (53, 430),
        "du2T_k0": (128, 430), "du2T_k1": (53, 430),
        "w2bT_k0": (128, 512), "w2bT_k1": (128, 512),
        "w2bT_k2": (128, 512), "w2bT_k3": (46, 512),
        "w3aT_k0": (128, 595), "w3aT_k1": (53, 595), "w3aT_k2": (128, 595),
        "w3aT_k3": (128, 595), "w3aT_k4": (128, 595), "w3aT_k5": (128, 595),
        "w3bT_k0": (128, 512), "w3bT_k1": (128, 512), "w3bT_k2": (128, 512),
        "w3bT_k3": (128, 512), "w3bT_k4": (83, 512),
        "g1a": (152, 1), "be1a": (152, 1), "g1b": (181, 1), "be1b": (181, 1),
        "g2a": (430, 1), "be2a": (430, 1), "g2b": (512, 1), "be2b": (512, 1),
        "g3a": (595, 1), "be3a": (595, 1), "g3b": (512, 1), "be3b": (512, 1),
        "E": (32, CB),
    }
    nc._ext_inputs = {}
    for nm, shp in in_specs.items():
        nc._ext_inputs[nm] = nc.dram_tensor(nm, list(shp), dt.float32,
                                            kind="ExternalInput").ap()
    nc._ext_outputs = {
        "out": nc.dram_tensor("out", [512, N], dt.float32,
                              kind="ExternalOutput").ap()}
    with tile.TileContext(nc) as tc:
        build_kernel(nc, tc)
    return nc


def _host_inputs(x, c1w1, c1g1, c1be1, c1w2, c1g2, c1be2,
                 c2w1, c2g1, c2be1, c2w2, c2g2, c2be2,
                 c3w1, c3g1, c3be1, c3w2, c3g2, c3be2):
    f32 = np.float32
    W1 = np.asarray(c1w1, f32)            # [152, 128]
    W1a, W1b = W1[:, :64], W1[:, 64:]
    W2 = np.asarray(c2w1, f32)            # [430, 362]
    W2a, W2b = W2[:, :181], W2[:, 181:]
    ws = {
        "w1aT": np.ascontiguousarray(W1a.T),
        "du1T": np.ascontiguousarray((W1b - W1a).T),
    }
    w1bT = np.ascontiguousarray(np.asarray(c1w2, f32).T)     # [152, 181]
    ws["w1bT_k0"], ws["w1bT_k1"] = w1bT[:128], w1bT[128:]
    w2aT = np.ascontiguousarray(W2a.T)                        # [181, 430]
    du2T = np.ascontiguousarray((W2b - W2a).T)
    ws["w2aT_k0"], ws["w2aT_k1"] = w2aT[:128], w2aT[128:]
    ws["du2T_k0"], ws["du2T_k1"] = du2T[:128], du2T[128:]
    w2bT = np.ascontiguousarray(np.asarray(c2w2, f32).T)     # [430, 512]
    for i, sl in enumerate((slice(0, 128), slice(128, 256), slice(256, 384),
                            slice(384, 430))):
        ws[f"w2bT_k{i}"] = w2bT[sl]
    w3aT = np.ascontiguousarray(np.asarray(c3w1, f32).T)     # [693, 595]
    cuts = (0, 128, 181, 309, 437, 565, 693)
    for i in range(6):
        ws[f"w3aT_k{i}"] = w3aT[cuts[i]:cuts[i + 1]]
    w3bT = np.ascontiguousarray(np.asarray(c3w2, f32).T)     # [595, 512]
    for i in range(5):
        ws[f"w3bT_k{i}"] = w3bT[128 * i:min(128 * (i + 1), 595)]
    for nm, v in (("g1a", c1g1), ("be1a", c1be1), ("g1b", c1g2), ("be1b", c1be2),
                  ("g2a", c2g1), ("be2a", c2be1), ("g2b", c2g2), ("be2b", c2be2),
                  ("g3a", c3g1), ("be3a", c3be1), ("g3b", c3g2), ("be3b", c3be2)):
        ws[nm] = np.ascontiguousarray(np.asarray(v, f32).reshape(-1, 1))
    E = np.zeros((32, CB), f32)
    for r in range(32):
        E[r, KNN * r:KNN * (r + 1)] = 1.0
    ws["E"] = E
    ws = {k: np.ascontiguousarray(v, f32) for k, v in ws.items()}
    xs = np.asarray(x, f32)
    in_maps = []
    for i in range(NCORES):
        m = dict(ws)
        m["x"] = np.ascontiguousarray(xs[i])
        in_maps.append(m)
    return in_maps


def kernel(x, c1w1, c1b1, c1g1, c1be1, c1w2, c1b2, c1g2, c1be2,
           c2w1, c2b1, c2g1, c2be1, c2w2, c2b2, c2g2, c2be2,
           c3w1, c3b1, c3g1, c3be1, c3w2, c3b2, c3g2, c3be2):
    # conv biases are absorbed exactly by the following BatchNorm (shift
    # invariance), so b* inputs are unused by the device program.
    if "nc" not in _CACHE:
        _CACHE["nc"] = _build()
    nc = _CACHE["nc"]
    in_maps = _host_inputs(x, c1w1, c1g1, c1be1, c1w2, c1g2, c1be2,
                           c2w1, c2g1, c2be1, c2w2, c2g2, c2be2,
                           c3w1, c3g1, c3be1, c3w2, c3g2, c3be2)
    res = run_bass_kernel_spmd(nc, in_maps, core_ids=list(range(NCORES)))
    _CACHE["last_result"] = res
    out = np.stack([res.results[i]["out"] for i in range(NCORES)], axis=0)
    return out.astype(np.float32)

